# revision 1
# baseline (speedup 1.0000x reference)
"""Trainium2 Bass kernel for nn_AgnisV5: 2-layer GPT encoder + gated
hierarchical recurrence + lm_head, SPMD over 8 NeuronCores.

Strategy: encoder + recurrence replicated on all cores (no collectives);
lm_head vocab-sharded 8 ways. Forward-pass simplification: stop_gradient is
identity, so the "settled" hierarchy path equals the differentiable shadow
path and blend == core_sh.

Layouts: feature-major activations [128p, K, N] (feature f = k*128 + p).
Encoder token columns are b-major (n = b*64 + t); recurrence/H/lm_head use
t-major (n = t*16 + b) via strided views of ctx.
"""
import numpy as np
import ml_dtypes
from contextlib import ExitStack

import concourse.bass as bass
import concourse.mybir as mybir
import concourse.tile as tile
from concourse import bacc
from concourse.bass_utils import run_bass_kernel_spmd

AF = mybir.ActivationFunctionType
OP = mybir.AluOpType
BF = mybir.dt.float16
F32 = mybir.dt.float32

D, HID, FFN, NH, NL, T, V, B = 768, 3072, 2048, 8, 2, 64, 50257, 16
HD = 96
KD = D // 128          # 6
MH = HID // 128        # 24
MF = FFN // 128        # 16
NTOK = B * T           # 1024
MT = NTOK // 128       # 8
VP = 53248             # padded vocab = 8*6656
VS = VP // 8           # 6656 per core
NCH = VS // 512        # 13
ALPHA = 0.4
QK_SCALE = 1.0 / np.sqrt(96.0)

bfnp = np.float16


def bc(ap, reps, pos):
    """Insert a stride-0 (broadcast) free dim at position pos (1-based over free dims)."""
    newap = list(ap.ap)
    newap.insert(pos, [0, reps])
    return bass.AP(tensor=ap.tensor, offset=ap.offset, ap=newap)


def build_nc():
    nc = bacc.Bacc(None, target_bir_lowering=False)

    # ---- DRAM I/O ----
    x0T = nc.dram_tensor("x0T", [D, NTOK], BF, kind="ExternalInput")
    enc_in = {}
    for l in range(NL):
        enc_in[f"wqk{l}"] = nc.dram_tensor(f"wqk{l}", [D, 2048], BF, kind="ExternalInput")
        enc_in[f"wv{l}"] = nc.dram_tensor(f"wv{l}", [D, 1024], BF, kind="ExternalInput")
        enc_in[f"wao{l}"] = nc.dram_tensor(f"wao{l}", [1024, D], BF, kind="ExternalInput")
        enc_in[f"ff1T{l}"] = nc.dram_tensor(f"ff1T{l}", [D, FFN], BF, kind="ExternalInput")
        enc_in[f"ff2T{l}"] = nc.dram_tensor(f"ff2T{l}", [FFN, D], BF, kind="ExternalInput")
        enc_in[f"bqk{l}"] = nc.dram_tensor(f"bqk{l}", [128, 16], F32, kind="ExternalInput")
        enc_in[f"bv{l}"] = nc.dram_tensor(f"bv{l}", [1, 1024], BF, kind="ExternalInput")
        enc_in[f"bao{l}"] = nc.dram_tensor(f"bao{l}", [128, KD], F32, kind="ExternalInput")
        enc_in[f"bff1{l}"] = nc.dram_tensor(f"bff1{l}", [128, MF], F32, kind="ExternalInput")
        enc_in[f"bff2{l}"] = nc.dram_tensor(f"bff2{l}", [128, KD], F32, kind="ExternalInput")
        enc_in[f"g1{l}"] = nc.dram_tensor(f"g1{l}", [128, KD], F32, kind="ExternalInput")
        enc_in[f"bb1{l}"] = nc.dram_tensor(f"bb1{l}", [128, KD], F32, kind="ExternalInput")
        enc_in[f"g2{l}"] = nc.dram_tensor(f"g2{l}", [128, KD], F32, kind="ExternalInput")
        enc_in[f"bb2{l}"] = nc.dram_tensor(f"bb2{l}", [128, KD], F32, kind="ExternalInput")
    encg = nc.dram_tensor("encg", [128, KD], F32, kind="ExternalInput")
    encb = nc.dram_tensor("encb", [128, KD], F32, kind="ExternalInput")
    maskT = nc.dram_tensor("maskT", [64, 64], BF, kind="ExternalInput")
    V0d = nc.dram_tensor("V0d", [D, HID], BF, kind="ExternalInput")
    V1d = nc.dram_tensor("V1d", [HID, D], BF, kind="ExternalInput")
    RRTd = nc.dram_tensor("RRTd", [D, 2 * D], BF, kind="ExternalInput")
    cw1d = nc.dram_tensor("cw1d", [D, D], BF, kind="ExternalInput")
    cw2d = nc.dram_tensor("cw2d", [D, D], BF, kind="ExternalInput")
    gw1d = nc.dram_tensor("gw1d", [D, D], BF, kind="ExternalInput")
    gw2d = nc.dram_tensor("gw2d", [D, D], BF, kind="ExternalInput")
    b0d = nc.dram_tensor("b0d", [128, MH], F32, kind="ExternalInput")
    b1d = nc.dram_tensor("b1d", [128, KD], F32, kind="ExternalInput")
    cb1d = nc.dram_tensor("cb1d", [128, KD], F32, kind="ExternalInput")
    cb2d = nc.dram_tensor("cb2d", [128, KD], F32, kind="ExternalInput")
    gbd = nc.dram_tensor("gbd", [128, KD], F32, kind="ExternalInput")
    outgd = nc.dram_tensor("outgd", [128, KD * B], F32, kind="ExternalInput")
    outbd = nc.dram_tensor("outbd", [128, KD * B], F32, kind="ExternalInput")
    lmTd = nc.dram_tensor("lmTd", [D, VS], BF, kind="ExternalInput")
    outd = nc.dram_tensor("out", [NTOK, VS], F32, kind="ExternalOutput")

    def r3(h, p=128):
        return h[:, :].rearrange("(k p) n -> p k n", p=p)

    with ExitStack() as ctx:
        tc = ctx.enter_context(tile.TileContext(nc))
        cross = ctx.enter_context(tc.tile_pool(name="cross", bufs=1))

        # constants (cross-scope)
        ones_col = cross.tile([128, 1], BF, tag="ones_col")
        nc.vector.memset(ones_col, 1.0)
        ones64 = cross.tile([64, 1], BF, tag="ones64")
        nc.vector.memset(ones64, 1.0)
        ones_row = cross.tile([1, 128], BF, tag="ones_row")
        nc.vector.memset(ones_row, 1.0)
        mask_sb = cross.tile([64, 64], BF, tag="mask")
        nc.sync.dma_start(out=mask_sb, in_=maskT[:, :])
        eps5 = cross.tile([128, 1], F32, tag="eps5")
        nc.vector.memset(eps5, 1e-5)
        eps24 = cross.tile([128, 1], F32, tag="eps24")
        nc.vector.memset(eps24, 1e-24)
        ctxF = cross.tile([128, KD, NTOK], BF, tag="ctxF")
        gctx = cross.tile([128, KD, NTOK], BF, tag="gctx")

        # ================= ENCODER SCOPE =================
        with tc.tile_pool(name="wenc", bufs=1) as wenc, \
             tc.tile_pool(name="bige", bufs=1) as bige, \
             tc.tile_pool(name="sme", bufs=1) as sme, \
             tc.tile_pool(name="pmm", bufs=2, space="PSUM") as pmm, \
             tc.tile_pool(name="pst", bufs=1, space="PSUM") as pst, \
             tc.tile_pool(name="pbc", bufs=2, space="PSUM") as pbc:

            def ln_fm(x_sb, g_ap, b_ap, out, l2_out=None):
                """x_sb [128, KD, 1024] bf16 -> LN -> out; optional l2 -> l2_out."""
                for j in range(2):
                    jj = slice(j * 512, (j + 1) * 512)
                    s1 = pst.tile([1, 512], F32, tag="s1")
                    s2 = pst.tile([1, 512], F32, tag="s2")
                    for k in range(KD):
                        sq = bige.tile([128, 512], BF, tag="sqc", bufs=2)
                        nc.vector.tensor_mul(sq, x_sb[:, k, jj], x_sb[:, k, jj])
                        nc.tensor.matmul(s1, ones_col, x_sb[:, k, jj],
                                         start=(k == 0), stop=(k == KD - 1))
                        nc.tensor.matmul(s2, ones_col, sq,
                                         start=(k == 0), stop=(k == KD - 1))
                    m = sme.tile([1, 512], F32, tag="m")
                    nc.vector.tensor_scalar_mul(m, s1, 1.0 / D)
                    ms = sme.tile([1, 512], F32, tag="ms")
                    nc.vector.tensor_scalar_mul(ms, s2, 1.0 / D)
                    mm2 = sme.tile([1, 512], F32, tag="mm2")
                    nc.vector.tensor_mul(mm2, m, m)
                    nc.vector.tensor_sub(ms, ms, mm2)
                    sd = sme.tile([1, 512], F32, tag="sd")
                    nc.scalar.activation(sd, ms, AF.Sqrt, bias=eps5[:1, :])
                    nc.vector.reciprocal(sd, sd)
                    ac16 = sme.tile([1, 2, 512], BF, tag="ac16")
                    nc.vector.tensor_copy(ac16[:, 0, :], sd)
                    nc.vector.tensor_mul(mm2, m, sd)
                    nc.vector.tensor_scalar_mul(ac16[:, 1, :], mm2, -1.0)
                    pa = pbc.tile([128, 512], F32, tag="pabc")
                    nc.tensor.matmul(pa, ones_row, ac16[:, 0, :], start=True, stop=True)
                    pc = pbc.tile([128, 512], F32, tag="pabc")
                    nc.tensor.matmul(pc, ones_row, ac16[:, 1, :], start=True, stop=True)
                    for k in range(KD):
                        t1 = bige.tile([128, 512], F32, tag="t1", bufs=2)
                        nc.vector.tensor_mul(t1, x_sb[:, k, jj], pa)
                        nc.vector.tensor_add(t1, t1, pc)
                        nc.vector.tensor_scalar(out[:, k, jj], t1, g_ap[:, k:k + 1],
                                                b_ap[:, k:k + 1], OP.mult, OP.add)
                if l2_out is None:
                    return
                for j in range(2):
                    jj = slice(j * 512, (j + 1) * 512)
                    s1 = pst.tile([1, 512], F32, tag="s1")
                    for k in range(KD):
                        sq = bige.tile([128, 512], BF, tag="sqc", bufs=2)
                        nc.vector.tensor_mul(sq, out[:, k, jj], out[:, k, jj])
                        nc.tensor.matmul(s1, ones_col, sq,
                                         start=(k == 0), stop=(k == KD - 1))
                    sd = sme.tile([1, 512], F32, tag="sd")
                    nc.scalar.activation(sd, s1, AF.Sqrt, bias=eps24[:1, :])
                    nc.vector.reciprocal(sd, sd)
                    rr16 = sme.tile([1, 512], BF, tag="rr16")
                    nc.vector.tensor_copy(rr16, sd)
                    pa = pbc.tile([128, 512], F32, tag="pabc")
                    nc.tensor.matmul(pa, ones_row, rr16, start=True, stop=True)
                    for k in range(KD):
                        nc.vector.tensor_mul(l2_out[:, k, jj], out[:, k, jj], pa)

            X = bige.tile([128, KD, NTOK], BF, tag="X")
            nc.sync.dma_start(out=X, in_=r3(x0T))
            gA = sme.tile([128, NL, KD], F32, tag="gA")
            bA = sme.tile([128, NL, KD], F32, tag="bA")
            gB = sme.tile([128, NL, KD], F32, tag="gB")
            bB = sme.tile([128, NL, KD], F32, tag="bB")
            for l in range(NL):
                nc.sync.dma_start(out=gA[:, l, :], in_=enc_in[f"g1{l}"][:, :])
                nc.sync.dma_start(out=bA[:, l, :], in_=enc_in[f"bb1{l}"][:, :])
                nc.sync.dma_start(out=gB[:, l, :], in_=enc_in[f"g2{l}"][:, :])
                nc.sync.dma_start(out=bB[:, l, :], in_=enc_in[f"bb2{l}"][:, :])

            for l in range(NL):
                XN = bige.tile([128, KD, NTOK], BF, tag="XN")
                ln_fm(X, gA[:, l, :], bA[:, l, :], XN)
                wqk = wenc.tile([128, KD, 2048], BF, tag="wbig")
                nc.sync.dma_start(out=wqk, in_=r3(enc_in[f"wqk{l}"]))
                bqk = sme.tile([128, 16], F32, tag="bqk")
                nc.sync.dma_start(out=bqk, in_=enc_in[f"bqk{l}"][:, :])
                # v projection (token-major)
                wv = wenc.tile([128, KD, 1024], BF, tag="wsmall")
                nc.sync.dma_start(out=wv, in_=r3(enc_in[f"wv{l}"]))
                bvr = sme.tile([1, 1024], BF, tag="bvr")
                nc.sync.dma_start(out=bvr, in_=enc_in[f"bv{l}"][:, :])
                bvs = bige.tile([128, 1024], BF, tag="bvs")
                for j in range(2):
                    pb = pbc.tile([128, 512], F32, tag="pabc")
                    nc.tensor.matmul(pb, ones_row, bvr[:, j * 512:(j + 1) * 512],
                                     start=True, stop=True)
                    nc.vector.tensor_copy(bvs[:, j * 512:(j + 1) * 512], pb)
                vS = bige.tile([64, B, 1024], BF, tag="vS")
                for b in range(B):
                    ps = pmm.tile([64, 1024], F32, tag="pmm")
                    for k in range(KD):
                        for j in range(2):
                            nc.tensor.matmul(ps[:, j * 512:(j + 1) * 512],
                                             XN[:, k, b * 64:(b + 1) * 64],
                                             wv[:, k, j * 512:(j + 1) * 512],
                                             start=(k == 0), stop=(k == KD - 1))
                    nc.vector.tensor_add(vS[:, b, :], ps, bvs[:64, :])
                # attention per head
                ofS = bige.tile([128, NH, NTOK], BF, tag="ofS")
                for h in range(NH):
                    qh = bige.tile([128, NTOK], BF, tag="qh", bufs=1)
                    ph = pmm.tile([128, 1024], F32, tag="pmm")
                    for k in range(KD):
                        for j in range(2):
                            nc.tensor.matmul(ph[:, j * 512:(j + 1) * 512],
                                             wqk[:, k, h * 128:(h + 1) * 128],
                                             XN[:, k, j * 512:(j + 1) * 512],
                                             start=(k == 0), stop=(k == KD - 1))
                    nc.vector.tensor_scalar_add(qh, ph, bqk[:, h:h + 1])
                    kh = bige.tile([128, NTOK], BF, tag="kh", bufs=1)
                    ph2 = pmm.tile([128, 1024], F32, tag="pmm")
                    for k in range(KD):
                        for j in range(2):
                            nc.tensor.matmul(ph2[:, j * 512:(j + 1) * 512],
                                             wqk[:, k, 1024 + h * 128:1024 + (h + 1) * 128],
                                             XN[:, k, j * 512:(j + 1) * 512],
                                             start=(k == 0), stop=(k == KD - 1))
                    nc.vector.tensor_scalar_add(kh, ph2, bqk[:, 8 + h:9 + h])
                    sT = pmm.tile([64, 1024], F32, tag="pmm")
                    for b in range(B):
                        nc.tensor.matmul(sT[:, b * 64:(b + 1) * 64],
                                         kh[:, b * 64:(b + 1) * 64],
                                         qh[:, b * 64:(b + 1) * 64],
                                         start=True, stop=True)
                    eT = bige.tile([64, NTOK], BF, tag="eT", bufs=2)
                    nc.scalar.activation(eT, sT, AF.Exp, scale=QK_SCALE)
                    nc.vector.tensor_mul(eT, eT, bc(mask_sb[:, :], B, 1))
                    rss = bige.tile([128, NTOK], BF, tag="rss", bufs=2)
                    for j in range(2):
                        jj = slice(j * 512, (j + 1) * 512)
                        ssum = pst.tile([1, 512], F32, tag="s1")
                        nc.tensor.matmul(ssum, ones64, eT[:, jj], start=True, stop=True)
                        rs = sme.tile([1, 512], F32, tag="m")
                        nc.vector.reciprocal(rs, ssum)
                        rs16 = sme.tile([1, 512], BF, tag="rr16")
                        nc.vector.tensor_copy(rs16, rs)
                        pb = pbc.tile([128, 512], F32, tag="pabc")
                        nc.tensor.matmul(pb, ones_row, rs16, start=True, stop=True)
                        nc.vector.tensor_copy(rss[:, jj], pb)
                    oh = pmm.tile([128, 1024], F32, tag="pmm")
                    for b in range(B):
                        nc.tensor.matmul(oh[:, b * 64:(b + 1) * 64],
                                         vS[:, b, h * 128:(h + 1) * 128],
                                         eT[:, b * 64:(b + 1) * 64],
                                         start=True, stop=True)
                    nc.vector.tensor_mul(ofS[:, h, :], oh, rss)
                # attn out + residual (in place on X)
                wao = wenc.tile([128, NH, D], BF, tag="wsmall")
                nc.sync.dma_start(out=wao,
                                  in_=enc_in[f"wao{l}"][:, :].rearrange("(k p) n -> p k n", p=128))
                bao = sme.tile([128, KD], F32, tag="bao")
                nc.sync.dma_start(out=bao, in_=enc_in[f"bao{l}"][:, :])
                for mo in range(KD):
                    ps = pmm.tile([128, 1024], F32, tag="pmm")
                    for kv in range(NH):
                        for j in range(2):
                            nc.tensor.matmul(ps[:, j * 512:(j + 1) * 512],
                                             wao[:, kv, mo * 128:(mo + 1) * 128],
                                             ofS[:, kv, j * 512:(j + 1) * 512],
                                             start=(kv == 0), stop=(kv == NH - 1))
                    t2 = bige.tile([128, 1024], F32, tag="t2", bufs=2)
                    nc.vector.tensor_scalar_add(t2, ps, bao[:, mo:mo + 1])
                    nc.vector.tensor_add(X[:, mo, :], X[:, mo, :], t2)
                # ffn
                XN2 = bige.tile([128, KD, NTOK], BF, tag="XN")
                ln_fm(X, gB[:, l, :], bB[:, l, :], XN2)
                w1 = wenc.tile([128, KD, FFN], BF, tag="wbig")
                nc.sync.dma_start(out=w1, in_=r3(enc_in[f"ff1T{l}"]))
                bf1 = sme.tile([128, MF], F32, tag="bf1")
                nc.sync.dma_start(out=bf1, in_=enc_in[f"bff1{l}"][:, :])
                G = bige.tile([128, MF, NTOK], BF, tag="G")
                for mf in range(MF):
                    ps = pmm.tile([128, 1024], F32, tag="pmm")
                    for k in range(KD):
                        for j in range(2):
                            nc.tensor.matmul(ps[:, j * 512:(j + 1) * 512],
                                             w1[:, k, mf * 128:(mf + 1) * 128],
                                             XN2[:, k, j * 512:(j + 1) * 512],
                                             start=(k == 0), stop=(k == KD - 1))
                    nc.scalar.activation(G[:, mf, :], ps, AF.Gelu, bias=bf1[:, mf:mf + 1])
                w2 = wenc.tile([128, MF, D], BF, tag="wbig")
                nc.sync.dma_start(out=w2, in_=r3(enc_in[f"ff2T{l}"]))
                bf2 = sme.tile([128, KD], F32, tag="bao")
                nc.sync.dma_start(out=bf2, in_=enc_in[f"bff2{l}"][:, :])
                for mo in range(KD):
                    ps = pmm.tile([128, 1024], F32, tag="pmm")
                    for kf in range(MF):
                        for j in range(2):
                            nc.tensor.matmul(ps[:, j * 512:(j + 1) * 512],
                                             w2[:, kf, mo * 128:(mo + 1) * 128],
                                             G[:, kf, j * 512:(j + 1) * 512],
                                             start=(kf == 0), stop=(kf == MF - 1))
                    t2 = bige.tile([128, 1024], F32, tag="t2", bufs=2)
                    nc.vector.tensor_scalar_add(t2, ps, bf2[:, mo:mo + 1])
                    nc.vector.tensor_add(X[:, mo, :], X[:, mo, :], t2)

            # final norm + l2 -> ctxF (cross pool)
            eg = sme.tile([128, KD], F32, tag="eg")
            eb = sme.tile([128, KD], F32, tag="eb")
            nc.sync.dma_start(out=eg, in_=encg[:, :])
            nc.sync.dma_start(out=eb, in_=encb[:, :])
            pre = bige.tile([128, KD, NTOK], BF, tag="XN")
            ln_fm(X, eg, eb, pre, l2_out=ctxF)

            # gate-ctx precompute -> gctx (cross pool)
            gw1 = wenc.tile([128, KD, D], BF, tag="wsmall")
            nc.sync.dma_start(out=gw1, in_=r3(gw1d))
            gbS = sme.tile([128, KD], F32, tag="gbS")
            nc.sync.dma_start(out=gbS, in_=gbd[:, :])
            for mo in range(KD):
                ps = pmm.tile([128, 1024], F32, tag="pmm")
                for k in range(KD):
                    for j in range(2):
                        nc.tensor.matmul(ps[:, j * 512:(j + 1) * 512],
                                         gw1[:, k, mo * 128:(mo + 1) * 128],
                                         ctxF[:, k, j * 512:(j + 1) * 512],
                                         start=(k == 0), stop=(k == KD - 1))
                nc.vector.tensor_scalar_add(gctx[:, mo, :], ps, gbS[:, mo:mo + 1])

        # ================= RECURRENCE SCOPE =================
        ctx_r = ctxF[:, :, :].rearrange("p k (b t) -> p t k b", b=B)
        gctx_r = gctx[:, :, :].rearrange("p k (b t) -> p t k b", b=B)
        with tc.tile_pool(name="wrec", bufs=1) as wrec, \
             tc.tile_pool(name="smr", bufs=2) as smr, \
             tc.tile_pool(name="prc", bufs=1, space="PSUM") as prc, \
             tc.tile_pool(name="pr6", bufs=1, space="PSUM") as pr6, \
             tc.tile_pool(name="prs", bufs=1, space="PSUM") as prs, \
             tc.tile_pool(name="pou", bufs=1, space="PSUM") as pou:

            V0w = wrec.tile([128, KD, HID], BF, tag="V0w")
            nc.sync.dma_start(out=V0w, in_=r3(V0d))
            V1w = wrec.tile([128, MH, D], BF, tag="V1w")
            nc.sync.dma_start(out=V1w, in_=r3(V1d))
            RRTw = wrec.tile([128, KD, 2 * D], BF, tag="RRTw")
            nc.sync.dma_start(out=RRTw, in_=r3(RRTd))
            cw1 = wrec.tile([128, KD, D], BF, tag="cw1")
            nc.sync.dma_start(out=cw1, in_=r3(cw1d))
            cw2 = wrec.tile([128, KD, D], BF, tag="cw2")
            nc.sync.dma_start(out=cw2, in_=r3(cw2d))
            gw2 = wrec.tile([128, KD, D], BF, tag="gw2")
            nc.sync.dma_start(out=gw2, in_=r3(gw2d))
            b0row = smr.tile([1, MH, 128], BF, tag="b0row", bufs=1)
            _b0ap = b0d[:, :]
            nc.gpsimd.dma_start(out=b0row, in_=bass.AP(tensor=_b0ap.tensor, offset=0,
                                                       ap=[[0, 1], [1, MH], [MH, 128]]))
            ones16 = smr.tile([1, B], BF, tag="ones16", bufs=1)
            nc.vector.memset(ones16, 1.0)
            b1S = smr.tile([128, KD], F32, tag="b1S", bufs=1)
            nc.sync.dma_start(out=b1S, in_=b1d[:, :])
            cb1S = smr.tile([128, KD], F32, tag="cb1S", bufs=1)
            nc.sync.dma_start(out=cb1S, in_=cb1d[:, :])
            cb2S = smr.tile([128, KD], F32, tag="cb2S", bufs=1)
            nc.sync.dma_start(out=cb2S, in_=cb2d[:, :])
            outgS = smr.tile([128, KD, B], F32, tag="outgS", bufs=1)
            nc.sync.dma_start(out=outgS, in_=outgd[:, :].rearrange("p (k b) -> p k b", b=B))
            outbS = smr.tile([128, KD, B], F32, tag="outbS", bufs=1)
            nc.sync.dma_start(out=outbS, in_=outbd[:, :].rearrange("p (k b) -> p k b", b=B))
            Hc = [wrec.tile([128, KD, 128], BF, tag=f"H{mt}", name=f"H{mt}")
                  for mt in range(MT)]

            import os as _os
            T_RUN = int(_os.environ.get("T_TRUNC", T))
            if T_RUN < T:
                for Hcm in Hc:
                    nc.vector.memset(Hcm, 0.0)
            for t in range(T_RUN):
                ctx_t = ctx_r[:, t]
                if t > 0:
                    hprev = Hc[(t - 1) // 8][:, :, ((t - 1) % 8) * B:((t - 1) % 8) * B + B]
                    traw = prc.tile([128, 2 * KD, B], F32, tag="ptraw")
                    for m in range(2 * KD):
                        for k in range(KD):
                            nc.tensor.matmul(traw[:, m, :],
                                             RRTw[:, k, m * 128:(m + 1) * 128],
                                             hprev[:, k, :],
                                             start=(k == 0), stop=(k == KD - 1))
                    tcS = smr.tile([128, KD, B], BF, tag="tcS")
                    nc.vector.scalar_tensor_tensor(tcS, traw[:, 0:KD, :], ALPHA, ctx_t,
                                                   OP.mult, OP.add)
                    tfS = smr.tile([128, KD, B], F32, tag="tfS")
                    nc.vector.tensor_scalar_mul(tfS, traw[:, KD:2 * KD, :], ALPHA)
                else:
                    tcS = smr.tile([128, KD, B], BF, tag="tcS")
                    nc.vector.tensor_copy(tcS, ctx_t)
                    tfS = smr.tile([128, KD, B], F32, tag="tfS")
                    nc.vector.memset(tfS, 0.0)
                u = prc.tile([128, MH, B], F32, tag="pu")
                for m in range(MH):
                    for k in range(KD):
                        nc.tensor.matmul(u[:, m, :], V0w[:, k, m * 128:(m + 1) * 128],
                                         tcS[:, k, :], start=(k == 0), stop=False)
                    nc.tensor.matmul(u[:, m, :], b0row[:, m, :], ones16,
                                     start=False, stop=True)
                hsS = smr.tile([128, MH, B], BF, tag="hsS")
                nc.scalar.activation(hsS, u, AF.Gelu)
                cpr = pr6.tile([128, KD, B], F32, tag="p6")
                for m in range(KD):
                    for k in range(MH):
                        nc.tensor.matmul(cpr[:, m, :], V1w[:, k, m * 128:(m + 1) * 128],
                                         hsS[:, k, :], start=(k == 0), stop=(k == MH - 1))
                cpf = smr.tile([128, KD, B], F32, tag="cpf")
                for m in range(KD):
                    nc.scalar.activation(cpf[:, m, :], cpr[:, m, :], AF.Gelu,
                                         bias=b1S[:, m:m + 1])
                sqc = smr.tile([128, KD, B], BF, tag="sqc")
                nc.vector.tensor_mul(sqc, cpf, cpf)
                nrm = prs.tile([1, 2, B], F32, tag="prs")
                for k in range(KD):
                    nc.tensor.matmul(nrm[:, 0, :], ones_col, sqc[:, k, :],
                                     start=(k == 0), stop=(k == KD - 1))
                sdc = smr.tile([1, B], F32, tag="sdc")
                nc.scalar.activation(sdc, nrm[:, 0, :], AF.Sqrt, bias=eps24[:1, :])
                nc.vector.reciprocal(sdc, sdc)
                rr16c = smr.tile([1, B], BF, tag="rr16c")
                nc.vector.tensor_copy(rr16c, sdc)
                rbc = prs.tile([128, B], F32, tag="pbc")
                nc.tensor.matmul(rbc, ones_row, rr16c, start=True, stop=True)
                coreS = smr.tile([128, KD, B], BF, tag="coreS")
                nc.vector.tensor_mul(coreS, cpf, bc(rbc[:, :], KD, 1))
                gm = pr6.tile([128, KD, B], F32, tag="p6")
                for m in range(KD):
                    for k in range(KD):
                        nc.tensor.matmul(gm[:, m, :], cw1[:, k, m * 128:(m + 1) * 128],
                                         coreS[:, k, :], start=(k == 0), stop=(k == KD - 1))
                gmS = smr.tile([128, KD, B], BF, tag="gmS")
                for m in range(KD):
                    nc.scalar.activation(gmS[:, m, :], gm[:, m, :], AF.Gelu,
                                         bias=cb1S[:, m:m + 1])
                cfp = pr6.tile([128, KD, B], F32, tag="p6")
                for m in range(KD):
                    for k in range(KD):
                        nc.tensor.matmul(cfp[:, m, :], cw2[:, k, m * 128:(m + 1) * 128],
                                         gmS[:, k, :], start=(k == 0), stop=(k == KD - 1))
                cfF = smr.tile([128, KD, B], F32, tag="cfF")
                for m in range(KD):
                    nc.vector.tensor_scalar_add(cfF[:, m, :], cfp[:, m, :], cb2S[:, m:m + 1])
                cfS = smr.tile([128, KD, B], BF, tag="cfS")
                nc.vector.tensor_copy(cfS, cfF)
                gp = pr6.tile([128, KD, B], F32, tag="p6")
                for m in range(KD):
                    for k in range(KD):
                        nc.tensor.matmul(gp[:, m, :], gw2[:, k, m * 128:(m + 1) * 128],
                                         cfS[:, k, :], start=(k == 0), stop=(k == KD - 1))
                gs = smr.tile([128, KD, B], F32, tag="gs")
                nc.vector.tensor_add(gs, gp, gctx_r[:, t])
                gateS = smr.tile([128, KD, B], F32, tag="gateS")
                nc.scalar.activation(gateS, gs, AF.Sigmoid)
                a1 = smr.tile([128, KD, B], F32, tag="a1")
                nc.vector.tensor_add(a1, cfF, tfS)
                nc.vector.tensor_sub(a1, a1, ctx_t)
                nc.vector.tensor_mul(a1, a1, gateS)
                hp = smr.tile([128, KD, B], F32, tag="hp")
                nc.vector.tensor_add(hp, a1, ctx_t)
                hp16 = smr.tile([128, KD, B], BF, tag="hp16")
                nc.vector.tensor_copy(hp16, hp)
                sqh = smr.tile([128, KD, B], BF, tag="sqh")
                nc.vector.tensor_mul(sqh, hp, hp)
                hs1 = prs.tile([1, B], F32, tag="prs")
                hs2 = prs.tile([1, B], F32, tag="prs2")
                for k in range(KD):
                    nc.tensor.matmul(hs1, ones_col, hp16[:, k, :],
                                     start=(k == 0), stop=(k == KD - 1))
                    nc.tensor.matmul(hs2, ones_col, sqh[:, k, :],
                                     start=(k == 0), stop=(k == KD - 1))
                hm = smr.tile([1, B], F32, tag="hm")
                nc.vector.tensor_scalar_mul(hm, hs1, 1.0 / D)
                hms = smr.tile([1, B], F32, tag="hms")
                nc.vector.tensor_scalar_mul(hms, hs2, 1.0 / D)
                hm2 = smr.tile([1, B], F32, tag="hm2")
                nc.vector.tensor_mul(hm2, hm, hm)
                nc.vector.tensor_sub(hms, hms, hm2)
                hsd = smr.tile([1, B], F32, tag="hsd")
                nc.scalar.activation(hsd, hms, AF.Sqrt, bias=eps5[:1, :])
                nc.vector.reciprocal(hsd, hsd)
                hac = smr.tile([1, 2, B], BF, tag="hac")
                nc.vector.tensor_copy(hac[:, 0, :], hsd)
                nc.vector.tensor_mul(hm2, hm, hsd)
                nc.vector.tensor_scalar_mul(hac[:, 1, :], hm2, -1.0)
                hab = prs.tile([128, 2, B], F32, tag="pbc")
                nc.tensor.matmul(hab[:, :, :].rearrange("p a b -> p (a b)"), ones_row,
                                 hac[:, :, :].rearrange("p a b -> p (a b)"),
                                 start=True, stop=True)
                hn = smr.tile([128, KD, B], F32, tag="hn")
                nc.vector.tensor_mul(hn, hp, bc(hab[:, 0, :], KD, 1))
                nc.vector.tensor_add(hn, hn, bc(hab[:, 1, :], KD, 1))
                nc.vector.tensor_mul(hn, hn, outgS)
                nc.vector.tensor_add(hn, hn, outbS)
                nc.vector.tensor_scalar(Hc[t // 8][:, :, (t % 8) * B:(t % 8) * B + B],
                                        hn, 5.0, -5.0, OP.min, OP.max)

            # ---------------- lm head ----------------
            # vocab-chunk outer so each lm weight chunk is DMA'd once (10MB
            # instead of 82MB) and reused across all 8 token tiles.
            for j in range(NCH):
                lw = wrec.tile([128, KD, 512], BF, tag="lmw", bufs=3)
                nc.sync.dma_start(
                    out=lw,
                    in_=lmTd[:, j * 512:(j + 1) * 512].rearrange("(k p) n -> p k n", p=128))
                for mt in range((T_RUN + 7) // 8):
                    ps = pou.tile([128, 512], F32, tag="plm", bufs=2)
                    for k in range(KD):
                        nc.tensor.matmul(ps, Hc[mt][:, k, :], lw[:, k, :],
                                         start=(k == 0), stop=(k == KD - 1))
                    ot = smr.tile([128, 512], F32, tag="otile", bufs=3)
                    nc.vector.tensor_copy(ot, ps)
                    nc.sync.dma_start(out=outd[mt * 128:(mt + 1) * 128,
                                               j * 512:(j + 1) * 512], in_=ot)

    nc.finalize()
    return nc


_NC_CACHE = {}


def _get_nc():
    if "nc" not in _NC_CACHE:
        _NC_CACHE["nc"] = build_nc()
    return _NC_CACHE["nc"]


def _prep_inputs(inputs):
    f = lambda x: np.asarray(x, np.float32)
    tok = np.asarray(inputs["token_ids"]).astype(np.int64)
    emb, pos = f(inputs["emb"]), f(inputs["pos_emb"])
    x0 = emb[tok.reshape(-1)] + np.tile(pos[:T], (B, 1))
    com = {"x0T": x0.T.astype(bfnp)}
    aiw, aib = f(inputs["attn_in_w"]), f(inputs["attn_in_b"])
    aow, aob = f(inputs["attn_out_w"]), f(inputs["attn_out_b"])
    for l in range(NL):
        wqk = np.zeros((D, 2048), np.float32)
        bqk = np.zeros(2048, np.float32)
        wv = np.zeros((D, 1024), np.float32)
        bv = np.zeros(1024, np.float32)
        wao = np.zeros((1024, D), np.float32)
        for h in range(NH):
            wqk[:, h * 128:h * 128 + HD] = aiw[l, h * HD:(h + 1) * HD, :].T
            wqk[:, 1024 + h * 128:1024 + h * 128 + HD] = aiw[l, D + h * HD:D + (h + 1) * HD, :].T
            bqk[h * 128:h * 128 + HD] = aib[l, h * HD:(h + 1) * HD]
            bqk[1024 + h * 128:1024 + h * 128 + HD] = aib[l, D + h * HD:D + (h + 1) * HD]
            wv[:, h * 128:h * 128 + HD] = aiw[l, 2 * D + h * HD:2 * D + (h + 1) * HD, :].T
            bv[h * 128:h * 128 + HD] = aib[l, 2 * D + h * HD:2 * D + (h + 1) * HD]
            wao[h * 128:h * 128 + HD, :] = aow[l][:, h * HD:(h + 1) * HD].T
        com[f"wqk{l}"] = wqk.astype(bfnp)
        com[f"bqk{l}"] = bqk.reshape(16, 128).T.copy()
        com[f"wv{l}"] = wv.astype(bfnp)
        com[f"bv{l}"] = bv.reshape(1, 1024).astype(bfnp)
        com[f"wao{l}"] = wao.astype(bfnp)
        com[f"bao{l}"] = aob[l].reshape(KD, 128).T.copy()
        com[f"ff1T{l}"] = f(inputs["ff_w1"])[l].T.astype(bfnp).copy()
        com[f"bff1{l}"] = f(inputs["ff_b1"])[l].reshape(MF, 128).T.copy()
        com[f"ff2T{l}"] = f(inputs["ff_w2"])[l].T.astype(bfnp).copy()
        com[f"bff2{l}"] = f(inputs["ff_b2"])[l].reshape(KD, 128).T.copy()
        com[f"g1{l}"] = f(inputs["n1_g"])[l].reshape(KD, 128).T.copy()
        com[f"bb1{l}"] = f(inputs["n1_b"])[l].reshape(KD, 128).T.copy()
        com[f"g2{l}"] = f(inputs["n2_g"])[l].reshape(KD, 128).T.copy()
        com[f"bb2{l}"] = f(inputs["n2_b"])[l].reshape(KD, 128).T.copy()
    com["encg"] = f(inputs["enc_norm_g"]).reshape(KD, 128).T.copy()
    com["encb"] = f(inputs["enc_norm_b"]).reshape(KD, 128).T.copy()
    tk, tq = np.meshgrid(np.arange(64), np.arange(64), indexing="ij")
    com["maskT"] = (tk <= tq).astype(bfnp)
    com["V0d"] = f(inputs["V0"]).astype(bfnp)
    com["V1d"] = f(inputs["V1"]).astype(bfnp)
    R, tw = f(inputs["R"]), f(inputs["temp_w"])
    com["RRTd"] = np.concatenate([R, R @ tw.T], axis=1).astype(bfnp)
    com["cw1d"] = f(inputs["cp_w1"]).T.astype(bfnp).copy()
    com["cw2d"] = f(inputs["cp_w2"]).T.astype(bfnp).copy()
    gw = f(inputs["gate_w"])
    com["gw1d"] = gw[:, :D].T.astype(bfnp).copy()
    com["gw2d"] = gw[:, D:].T.astype(bfnp).copy()
    com["b0d"] = f(inputs["b0"]).reshape(MH, 128).T.copy()
    com["b1d"] = f(inputs["b1"]).reshape(KD, 128).T.copy()
    com["cb1d"] = f(inputs["cp_b1"]).reshape(KD, 128).T.copy()
    com["cb2d"] = f(inputs["cp_b2"]).reshape(KD, 128).T.copy()
    com["gbd"] = f(inputs["gate_b"]).reshape(KD, 128).T.copy()
    og = f(inputs["out_g"]).reshape(KD, 128).T          # [128, KD]
    ob = f(inputs["out_b"]).reshape(KD, 128).T
    com["outgd"] = np.repeat(og[:, :, None], B, axis=2).reshape(128, KD * B).copy()
    com["outbd"] = np.repeat(ob[:, :, None], B, axis=2).reshape(128, KD * B).copy()
    lmp = np.zeros((VP, D), np.float32)
    lmp[:V] = f(inputs["lm_head"])
    lmT = lmp.T.astype(bfnp)
    shards = [np.ascontiguousarray(lmT[:, c * VS:(c + 1) * VS]) for c in range(8)]
    return com, shards


LAST_RESULT = {}


def kernel(**inputs):
    import os
    nc = _get_nc()
    com, shards = _prep_inputs(inputs)
    in_maps = [{**com, "lmTd": shards[c]} for c in range(8)]
    kw = {}
    if os.environ.get("KTRACE"):
        kw = dict(trace=True, tmpdir=os.environ.get("KTRACE_DIR", "/root/problem/trace_out"))
    res = run_bass_kernel_spmd(nc, in_maps, core_ids=list(range(8)), **kw)
    LAST_RESULT["res"] = res
    parts = [res.results[c]["out"] for c in range(8)]          # each [1024, VS], t-major rows
    full = np.concatenate(parts, axis=1)[:, :V]                # [1024, V]
    return np.ascontiguousarray(full.reshape(T, B, V).transpose(1, 0, 2))



# revision 36
# speedup vs baseline: 1.3546x; 1.3546x over previous
"""Trainium2 Bass kernel for nn_AgnisV5: 2-layer GPT encoder + gated
hierarchical recurrence + lm_head, SPMD over 8 NeuronCores.

Strategy: encoder + recurrence replicated on all cores (no collectives);
lm_head vocab-sharded 8 ways. Forward-pass simplification: stop_gradient is
identity, so the "settled" hierarchy path equals the differentiable shadow
path and blend == core_sh.

Layouts: feature-major activations [128p, K, N] (feature f = k*128 + p).
Encoder token columns are b-major (n = b*64 + t); recurrence/H/lm_head use
t-major (n = t*16 + b) via strided views of ctx.
"""
import numpy as np
import ml_dtypes
from contextlib import ExitStack

import concourse.bass as bass
import concourse.mybir as mybir
import concourse.tile as tile
from concourse import bacc
from concourse.bass_utils import run_bass_kernel_spmd

AF = mybir.ActivationFunctionType
OP = mybir.AluOpType
BF = mybir.dt.float16
F32 = mybir.dt.float32

D, HID, FFN, NH, NL, T, V, B = 768, 3072, 2048, 8, 2, 64, 50257, 16
HD = 96
KD = D // 128          # 6
MH = HID // 128        # 24
MF = FFN // 128        # 16
NTOK = B * T           # 1024
MT = NTOK // 128       # 8
VP = 53248             # padded vocab = 8*6656
VS = VP // 8           # 6656 per core
NCH = VS // 512        # 13
ALPHA = 0.4
QK_SCALE = 1.0 / np.sqrt(96.0)

bfnp = np.float16


def bc(ap, reps, pos):
    """Insert a stride-0 (broadcast) free dim at position pos (1-based over free dims)."""
    newap = list(ap.ap)
    newap.insert(pos, [0, reps])
    return bass.AP(tensor=ap.tensor, offset=ap.offset, ap=newap)


def build_nc():
    nc = bacc.Bacc(None, target_bir_lowering=False)

    # ---- DRAM I/O ----
    x0T = nc.dram_tensor("x0T", [D, NTOK], BF, kind="ExternalInput")
    enc_in = {}
    for l in range(NL):
        enc_in[f"wqk{l}"] = nc.dram_tensor(f"wqk{l}", [D, 2048], BF, kind="ExternalInput")
        enc_in[f"wv{l}"] = nc.dram_tensor(f"wv{l}", [D, 1024], BF, kind="ExternalInput")
        enc_in[f"wao{l}"] = nc.dram_tensor(f"wao{l}", [1024, D], BF, kind="ExternalInput")
        enc_in[f"ff1T{l}"] = nc.dram_tensor(f"ff1T{l}", [D, FFN], BF, kind="ExternalInput")
        enc_in[f"ff2T{l}"] = nc.dram_tensor(f"ff2T{l}", [FFN, D], BF, kind="ExternalInput")
        enc_in[f"bqk{l}"] = nc.dram_tensor(f"bqk{l}", [128, 16], F32, kind="ExternalInput")
        enc_in[f"bv{l}"] = nc.dram_tensor(f"bv{l}", [1, 1024], BF, kind="ExternalInput")
        enc_in[f"bao{l}"] = nc.dram_tensor(f"bao{l}", [128, KD], F32, kind="ExternalInput")
        enc_in[f"bff1{l}"] = nc.dram_tensor(f"bff1{l}", [128, MF], F32, kind="ExternalInput")
        enc_in[f"bff2{l}"] = nc.dram_tensor(f"bff2{l}", [128, KD], F32, kind="ExternalInput")
        enc_in[f"g1{l}"] = nc.dram_tensor(f"g1{l}", [128, KD], F32, kind="ExternalInput")
        enc_in[f"bb1{l}"] = nc.dram_tensor(f"bb1{l}", [128, KD], F32, kind="ExternalInput")
        enc_in[f"g2{l}"] = nc.dram_tensor(f"g2{l}", [128, KD], F32, kind="ExternalInput")
        enc_in[f"bb2{l}"] = nc.dram_tensor(f"bb2{l}", [128, KD], F32, kind="ExternalInput")
    encg = nc.dram_tensor("encg", [128, KD], F32, kind="ExternalInput")
    encb = nc.dram_tensor("encb", [128, KD], F32, kind="ExternalInput")
    maskT = nc.dram_tensor("maskT", [64, 64], BF, kind="ExternalInput")
    V0d = nc.dram_tensor("V0d", [D, HID], BF, kind="ExternalInput")
    V1d = nc.dram_tensor("V1d", [HID, D], BF, kind="ExternalInput")
    Whud = nc.dram_tensor("Whud", [D, HID], BF, kind="ExternalInput")
    Wtfd = nc.dram_tensor("Wtfd", [D, D], BF, kind="ExternalInput")
    cw1d = nc.dram_tensor("cw1d", [D, D], BF, kind="ExternalInput")
    cw2d = nc.dram_tensor("cw2d", [D, D], BF, kind="ExternalInput")
    gw1d = nc.dram_tensor("gw1d", [D, D], BF, kind="ExternalInput")
    gw2d = nc.dram_tensor("gw2d", [D, D], BF, kind="ExternalInput")
    b0d = nc.dram_tensor("b0d", [128, MH], F32, kind="ExternalInput")
    b1d = nc.dram_tensor("b1d", [128, KD], F32, kind="ExternalInput")
    cb1d = nc.dram_tensor("cb1d", [128, KD], F32, kind="ExternalInput")
    cb2d = nc.dram_tensor("cb2d", [128, KD], F32, kind="ExternalInput")
    gbd = nc.dram_tensor("gbd", [128, KD], F32, kind="ExternalInput")
    outgd = nc.dram_tensor("outgd", [128, KD], F32, kind="ExternalInput")
    outbd = nc.dram_tensor("outbd", [128, KD], F32, kind="ExternalInput")
    lmTd = nc.dram_tensor("lmTd", [D, VS], BF, kind="ExternalInput")
    eyed = nc.dram_tensor("eyed", [128, 128], BF, kind="ExternalInput")
    outd = nc.dram_tensor("out", [NTOK, VS], F32, kind="ExternalOutput")
    import os as _os0
    KDEBUG = bool(_os0.environ.get("KDEBUG"))
    if KDEBUG:
        dbg_ctx = nc.dram_tensor("dbg_ctx", [128, KD * NTOK], BF, kind="ExternalOutput")
        dbg_uc = nc.dram_tensor("dbg_uc", [128, MH * 128], BF, kind="ExternalOutput")
        dbg_h = nc.dram_tensor("dbg_h", [128, KD * 128], BF, kind="ExternalOutput")
        dbg_st = {}
        for nm, width, dt_ in [("hs", MH * B, BF), ("cpf", KD * B, F32),
                               ("core", KD * B, BF), ("gm", KD * B, BF),
                               ("cf", KD * B, F32), ("tg", KD * B, F32),
                               ("hp", KD * B, F32), ("ln", KD * B, F32)]:
            dbg_st[nm] = nc.dram_tensor(f"dbg_{nm}", [128, width], dt_,
                                        kind="ExternalOutput")

    def r3(h, p=128):
        return h[:, :].rearrange("(k p) n -> p k n", p=p)

    with ExitStack() as ctx:
        tc = ctx.enter_context(tile.TileContext(nc))
        cross = ctx.enter_context(tc.tile_pool(name="cross", bufs=1))

        # constants (cross-scope)
        ones_col = cross.tile([128, 1], BF, tag="ones_col")
        nc.vector.memset(ones_col, 1.0)
        ones64 = cross.tile([64, 1], BF, tag="ones64")
        nc.vector.memset(ones64, 1.0)
        ones_row = cross.tile([1, 128], BF, tag="ones_row")
        nc.vector.memset(ones_row, 1.0)
        ones_row_f = cross.tile([1, 128], F32, tag="ones_row_f")
        nc.vector.memset(ones_row_f, 1.0)
        mask_sb = cross.tile([64, 64], BF, tag="mask")
        nc.sync.dma_start(out=mask_sb, in_=maskT[:, :])
        eps5 = cross.tile([128, 1], F32, tag="eps5")
        nc.vector.memset(eps5, 1e-5)
        eps24 = cross.tile([128, 1], F32, tag="eps24")
        nc.vector.memset(eps24, 1e-24)
        ctxF = cross.tile([128, KD, NTOK], BF, tag="ctxF")
        gctx = cross.tile([128, KD, NTOK], BF, tag="gctx")
        # DRAM scratch for uc (tile pool so RAW deps across DMAs are tracked)
        ucdram = ctx.enter_context(tc.tile_pool(name="ucdram", bufs=1, space="DRAM"))
        ucT = ucdram.tile([128, MH, NTOK], BF, tag="ucT")

        # ================= ENCODER SCOPE =================
        with tc.tile_pool(name="wenc", bufs=1) as wenc, \
             tc.tile_pool(name="bige", bufs=1) as bige, \
             tc.tile_pool(name="sme", bufs=1) as sme, \
             tc.tile_pool(name="pmm", bufs=2, space="PSUM") as pmm, \
             tc.tile_pool(name="pst", bufs=1, space="PSUM") as pst, \
             tc.tile_pool(name="pbc", bufs=2, space="PSUM") as pbc:

            def ln_fm(x_sb, g_ap, b_ap, out, l2_out=None):
                """x_sb [128, KD, 1024] bf16 -> LN -> out; optional l2 -> l2_out."""
                for j in range(2):
                    jj = slice(j * 512, (j + 1) * 512)
                    s1 = pst.tile([1, 512], F32, tag="s1")
                    s2 = pst.tile([1, 512], F32, tag="s2")
                    for k in range(KD):
                        sq = bige.tile([128, 512], BF, tag="sqc", bufs=2)
                        nc.vector.tensor_mul(sq, x_sb[:, k, jj], x_sb[:, k, jj])
                        nc.tensor.matmul(s1, ones_col, x_sb[:, k, jj],
                                         start=(k == 0), stop=(k == KD - 1))
                        nc.tensor.matmul(s2, ones_col, sq,
                                         start=(k == 0), stop=(k == KD - 1))
                    m = sme.tile([1, 512], F32, tag="m")
                    nc.vector.tensor_scalar_mul(m, s1, 1.0 / D)
                    ms = sme.tile([1, 512], F32, tag="ms")
                    nc.vector.tensor_scalar_mul(ms, s2, 1.0 / D)
                    mm2 = sme.tile([1, 512], F32, tag="mm2")
                    nc.vector.tensor_mul(mm2, m, m)
                    nc.vector.tensor_sub(ms, ms, mm2)
                    sd = sme.tile([1, 512], F32, tag="sd")
                    nc.scalar.activation(sd, ms, AF.Sqrt, bias=eps5[:1, :])
                    nc.vector.reciprocal(sd, sd)
                    ac16 = sme.tile([1, 2, 512], BF, tag="ac16")
                    nc.vector.tensor_copy(ac16[:, 0, :], sd)
                    nc.vector.tensor_mul(mm2, m, sd)
                    nc.vector.tensor_scalar_mul(ac16[:, 1, :], mm2, -1.0)
                    pa = pbc.tile([128, 512], F32, tag="pabc")
                    nc.tensor.matmul(pa, ones_row, ac16[:, 0, :], start=True, stop=True)
                    pc = pbc.tile([128, 512], F32, tag="pabc")
                    nc.tensor.matmul(pc, ones_row, ac16[:, 1, :], start=True, stop=True)
                    for k in range(KD):
                        t1 = bige.tile([128, 512], F32, tag="t1", bufs=2)
                        nc.vector.tensor_mul(t1, x_sb[:, k, jj], pa)
                        nc.vector.tensor_add(t1, t1, pc)
                        nc.vector.tensor_scalar(out[:, k, jj], t1, g_ap[:, k:k + 1],
                                                b_ap[:, k:k + 1], OP.mult, OP.add)
                if l2_out is None:
                    return
                for j in range(2):
                    jj = slice(j * 512, (j + 1) * 512)
                    s1 = pst.tile([1, 512], F32, tag="s1")
                    for k in range(KD):
                        sq = bige.tile([128, 512], BF, tag="sqc", bufs=2)
                        nc.vector.tensor_mul(sq, out[:, k, jj], out[:, k, jj])
                        nc.tensor.matmul(s1, ones_col, sq,
                                         start=(k == 0), stop=(k == KD - 1))
                    sd = sme.tile([1, 512], F32, tag="sd")
                    nc.scalar.activation(sd, s1, AF.Sqrt, bias=eps24[:1, :])
                    nc.vector.reciprocal(sd, sd)
                    rr16 = sme.tile([1, 512], BF, tag="rr16")
                    nc.vector.tensor_copy(rr16, sd)
                    pa = pbc.tile([128, 512], F32, tag="pabc")
                    nc.tensor.matmul(pa, ones_row, rr16, start=True, stop=True)
                    for k in range(KD):
                        nc.vector.tensor_mul(l2_out[:, k, jj], out[:, k, jj], pa)

            X = bige.tile([128, KD, NTOK], BF, tag="X")
            nc.sync.dma_start(out=X, in_=r3(x0T))
            gA = sme.tile([128, NL, KD], F32, tag="gA")
            bA = sme.tile([128, NL, KD], F32, tag="bA")
            gB = sme.tile([128, NL, KD], F32, tag="gB")
            bB = sme.tile([128, NL, KD], F32, tag="bB")
            for l in range(NL):
                nc.sync.dma_start(out=gA[:, l, :], in_=enc_in[f"g1{l}"][:, :])
                nc.sync.dma_start(out=bA[:, l, :], in_=enc_in[f"bb1{l}"][:, :])
                nc.sync.dma_start(out=gB[:, l, :], in_=enc_in[f"g2{l}"][:, :])
                nc.sync.dma_start(out=bB[:, l, :], in_=enc_in[f"bb2{l}"][:, :])

            for l in range(NL):
                XN = bige.tile([128, KD, NTOK], BF, tag="XN")
                ln_fm(X, gA[:, l, :], bA[:, l, :], XN)
                wqk = wenc.tile([128, KD, 2048], BF, tag="wbig")
                nc.sync.dma_start(out=wqk, in_=r3(enc_in[f"wqk{l}"]))
                bqk = sme.tile([128, 16], F32, tag="bqk")
                nc.sync.dma_start(out=bqk, in_=enc_in[f"bqk{l}"][:, :])
                # v projection (token-major)
                wv = wenc.tile([128, KD, 1024], BF, tag="wsmall")
                nc.sync.dma_start(out=wv, in_=r3(enc_in[f"wv{l}"]))
                bvr = sme.tile([1, 1024], BF, tag="bvr")
                nc.sync.dma_start(out=bvr, in_=enc_in[f"bv{l}"][:, :])
                bvs = bige.tile([128, 1024], BF, tag="bvs")
                for j in range(2):
                    pb = pbc.tile([128, 512], F32, tag="pabc")
                    nc.tensor.matmul(pb, ones_row, bvr[:, j * 512:(j + 1) * 512],
                                     start=True, stop=True)
                    nc.vector.tensor_copy(bvs[:, j * 512:(j + 1) * 512], pb)
                vS = bige.tile([64, B, 1024], BF, tag="vS")
                for b in range(B):
                    ps = pmm.tile([64, 1024], F32, tag="pmm")
                    for k in range(KD):
                        for j in range(2):
                            nc.tensor.matmul(ps[:, j * 512:(j + 1) * 512],
                                             XN[:, k, b * 64:(b + 1) * 64],
                                             wv[:, k, j * 512:(j + 1) * 512],
                                             start=(k == 0), stop=(k == KD - 1))
                    nc.vector.tensor_add(vS[:, b, :], ps, bvs[:64, :])
                # attention per head
                ofS = bige.tile([128, NH, NTOK], BF, tag="ofS")
                for h in range(NH):
                    qh = bige.tile([128, NTOK], BF, tag="qh", bufs=1)
                    ph = pmm.tile([128, 1024], F32, tag="pmm")
                    for k in range(KD):
                        for j in range(2):
                            nc.tensor.matmul(ph[:, j * 512:(j + 1) * 512],
                                             wqk[:, k, h * 128:(h + 1) * 128],
                                             XN[:, k, j * 512:(j + 1) * 512],
                                             start=(k == 0), stop=(k == KD - 1))
                    nc.vector.tensor_scalar_add(qh, ph, bqk[:, h:h + 1])
                    kh = bige.tile([128, NTOK], BF, tag="kh", bufs=1)
                    ph2 = pmm.tile([128, 1024], F32, tag="pmm")
                    for k in range(KD):
                        for j in range(2):
                            nc.tensor.matmul(ph2[:, j * 512:(j + 1) * 512],
                                             wqk[:, k, 1024 + h * 128:1024 + (h + 1) * 128],
                                             XN[:, k, j * 512:(j + 1) * 512],
                                             start=(k == 0), stop=(k == KD - 1))
                    nc.vector.tensor_scalar_add(kh, ph2, bqk[:, 8 + h:9 + h])
                    sT = pmm.tile([64, 1024], F32, tag="pmm")
                    for b in range(B):
                        nc.tensor.matmul(sT[:, b * 64:(b + 1) * 64],
                                         kh[:, b * 64:(b + 1) * 64],
                                         qh[:, b * 64:(b + 1) * 64],
                                         start=True, stop=True)
                    eT = bige.tile([64, NTOK], BF, tag="eT", bufs=2)
                    nc.scalar.activation(eT, sT, AF.Exp, scale=QK_SCALE)
                    nc.vector.tensor_mul(eT, eT, bc(mask_sb[:, :], B, 1))
                    rss = bige.tile([128, NTOK], BF, tag="rss", bufs=2)
                    for j in range(2):
                        jj = slice(j * 512, (j + 1) * 512)
                        ssum = pst.tile([1, 512], F32, tag="s1")
                        nc.tensor.matmul(ssum, ones64, eT[:, jj], start=True, stop=True)
                        rs = sme.tile([1, 512], F32, tag="m")
                        nc.vector.reciprocal(rs, ssum)
                        rs16 = sme.tile([1, 512], BF, tag="rr16")
                        nc.vector.tensor_copy(rs16, rs)
                        pb = pbc.tile([128, 512], F32, tag="pabc")
                        nc.tensor.matmul(pb, ones_row, rs16, start=True, stop=True)
                        nc.vector.tensor_copy(rss[:, jj], pb)
                    oh = pmm.tile([128, 1024], F32, tag="pmm")
                    for b in range(B):
                        nc.tensor.matmul(oh[:, b * 64:(b + 1) * 64],
                                         vS[:, b, h * 128:(h + 1) * 128],
                                         eT[:, b * 64:(b + 1) * 64],
                                         start=True, stop=True)
                    nc.vector.tensor_mul(ofS[:, h, :], oh, rss)
                # attn out + residual (in place on X)
                wao = wenc.tile([128, NH, D], BF, tag="wsmall")
                nc.sync.dma_start(out=wao,
                                  in_=enc_in[f"wao{l}"][:, :].rearrange("(k p) n -> p k n", p=128))
                bao = sme.tile([128, KD], F32, tag="bao")
                nc.sync.dma_start(out=bao, in_=enc_in[f"bao{l}"][:, :])
                for mo in range(KD):
                    ps = pmm.tile([128, 1024], F32, tag="pmm")
                    for kv in range(NH):
                        for j in range(2):
                            nc.tensor.matmul(ps[:, j * 512:(j + 1) * 512],
                                             wao[:, kv, mo * 128:(mo + 1) * 128],
                                             ofS[:, kv, j * 512:(j + 1) * 512],
                                             start=(kv == 0), stop=(kv == NH - 1))
                    t2 = bige.tile([128, 1024], F32, tag="t2", bufs=2)
                    nc.vector.tensor_scalar_add(t2, ps, bao[:, mo:mo + 1])
                    nc.vector.tensor_add(X[:, mo, :], X[:, mo, :], t2)
                # ffn
                XN2 = bige.tile([128, KD, NTOK], BF, tag="XN")
                ln_fm(X, gB[:, l, :], bB[:, l, :], XN2)
                w1 = wenc.tile([128, KD, FFN], BF, tag="wbig")
                nc.sync.dma_start(out=w1, in_=r3(enc_in[f"ff1T{l}"]))
                bf1 = sme.tile([128, MF], F32, tag="bf1")
                nc.sync.dma_start(out=bf1, in_=enc_in[f"bff1{l}"][:, :])
                G = bige.tile([128, MF, NTOK], BF, tag="G")
                for mf in range(MF):
                    ps = pmm.tile([128, 1024], F32, tag="pmm")
                    for k in range(KD):
                        for j in range(2):
                            nc.tensor.matmul(ps[:, j * 512:(j + 1) * 512],
                                             w1[:, k, mf * 128:(mf + 1) * 128],
                                             XN2[:, k, j * 512:(j + 1) * 512],
                                             start=(k == 0), stop=(k == KD - 1))
                    nc.scalar.activation(G[:, mf, :], ps, AF.Gelu, bias=bf1[:, mf:mf + 1])
                w2 = wenc.tile([128, MF, D], BF, tag="wbig")
                nc.sync.dma_start(out=w2, in_=r3(enc_in[f"ff2T{l}"]))
                bf2 = sme.tile([128, KD], F32, tag="bao")
                nc.sync.dma_start(out=bf2, in_=enc_in[f"bff2{l}"][:, :])
                for mo in range(KD):
                    ps = pmm.tile([128, 1024], F32, tag="pmm")
                    for kf in range(MF):
                        for j in range(2):
                            nc.tensor.matmul(ps[:, j * 512:(j + 1) * 512],
                                             w2[:, kf, mo * 128:(mo + 1) * 128],
                                             G[:, kf, j * 512:(j + 1) * 512],
                                             start=(kf == 0), stop=(kf == MF - 1))
                    t2 = bige.tile([128, 1024], F32, tag="t2", bufs=2)
                    nc.vector.tensor_scalar_add(t2, ps, bf2[:, mo:mo + 1])
                    nc.vector.tensor_add(X[:, mo, :], X[:, mo, :], t2)

            # final norm + l2 -> ctxF (cross pool)
            eg = sme.tile([128, KD], F32, tag="eg")
            eb = sme.tile([128, KD], F32, tag="eb")
            nc.sync.dma_start(out=eg, in_=encg[:, :])
            nc.sync.dma_start(out=eb, in_=encb[:, :])
            pre = bige.tile([128, KD, NTOK], BF, tag="XN")
            ln_fm(X, eg, eb, pre, l2_out=ctxF)

            # gate-ctx precompute -> gctx (cross pool)
            gw1 = wenc.tile([128, KD, D], BF, tag="wsmall")
            nc.sync.dma_start(out=gw1, in_=r3(gw1d))
            gbS = sme.tile([128, KD], F32, tag="gbS")
            nc.sync.dma_start(out=gbS, in_=gbd[:, :])
            for mo in range(KD):
                ps = pmm.tile([128, 1024], F32, tag="pmm")
                for k in range(KD):
                    for j in range(2):
                        nc.tensor.matmul(ps[:, j * 512:(j + 1) * 512],
                                         gw1[:, k, mo * 128:(mo + 1) * 128],
                                         ctxF[:, k, j * 512:(j + 1) * 512],
                                         start=(k == 0), stop=(k == KD - 1))
                nc.vector.tensor_scalar_add(gctx[:, mo, :], ps, gbS[:, mo:mo + 1])

        # ===== uc precompute: uc[:, t*16+b] = V0^T ctx[:, b*64+t] + b0 =====
        ctx_tb = ctxF[:, :, :].rearrange("p k (b t) -> p k t b", b=B)
        with tc.tile_pool(name="wuc", bufs=1) as wuc, \
             tc.tile_pool(name="sucs", bufs=2) as sucs, \
             tc.tile_pool(name="puc", bufs=1, space="PSUM") as puc:
            V0e = wuc.tile([128, KD, HID], BF, tag="V0e")
            nc.sync.dma_start(out=V0e, in_=r3(V0d))
            b0rowE = sucs.tile([1, MH, 128], BF, tag="b0rowE", bufs=1)
            _b0ap = b0d[:, :]
            nc.gpsimd.dma_start(out=b0rowE, in_=bass.AP(tensor=_b0ap.tensor, offset=0,
                                                        ap=[[0, 1], [1, MH], [MH, 128]]))
            ones128r = sucs.tile([1, 128], BF, tag="ones128r", bufs=1)
            nc.vector.memset(ones128r, 1.0)
            for mt in range(MT):
                pu8 = puc.tile([128, MH, 128], F32, tag="puc")
                for m in range(MH):
                    for k in range(KD):
                        nc.tensor.matmul(pu8[:, m, :], V0e[:, k, m * 128:(m + 1) * 128],
                                         ctx_tb[:, k, mt * 8:(mt + 1) * 8, :],
                                         start=(k == 0), stop=False)
                    nc.tensor.matmul(pu8[:, m, :], b0rowE[:, m, :], ones128r,
                                     start=False, stop=True)
                ucs = sucs.tile([128, MH, 128], BF, tag="ucs")
                nc.scalar.activation(ucs, pu8, AF.Copy)
                nc.sync.dma_start(out=ucT[:, :, mt * 128:(mt + 1) * 128], in_=ucs)

        # ================= RECURRENCE SCOPE =================
        ctx_r = ctxF[:, :, :].rearrange("p k (b t) -> p t k b", b=B)
        gctx_r = gctx[:, :, :].rearrange("p k (b t) -> p t k b", b=B)
        with tc.tile_pool(name="wrec", bufs=1) as wrec, \
             tc.tile_pool(name="smr", bufs=2) as smr, \
             tc.tile_pool(name="prc", bufs=1, space="PSUM") as prc, \
             tc.tile_pool(name="pr6", bufs=1, space="PSUM") as pr6, \
             tc.tile_pool(name="prs", bufs=1, space="PSUM") as prs, \
             tc.tile_pool(name="pou", bufs=1, space="PSUM") as pou:

            Whu = wrec.tile([128, KD, HID], BF, tag="Whu")
            nc.sync.dma_start(out=Whu, in_=r3(Whud))
            V1w = wrec.tile([128, MH, D], BF, tag="V1w")
            nc.sync.dma_start(out=V1w, in_=r3(V1d))
            Wtf = wrec.tile([128, KD, D], BF, tag="Wtf")
            nc.sync.dma_start(out=Wtf, in_=r3(Wtfd))
            cw1 = wrec.tile([128, KD, D], BF, tag="cw1")
            nc.sync.dma_start(out=cw1, in_=r3(cw1d))
            cw2 = wrec.tile([128, KD, D], BF, tag="cw2")
            nc.sync.dma_start(out=cw2, in_=r3(cw2d))
            gw2 = wrec.tile([128, KD, D], BF, tag="gw2")
            nc.sync.dma_start(out=gw2, in_=r3(gw2d))
            b1S = smr.tile([128, KD], F32, tag="b1S", bufs=1)
            nc.sync.dma_start(out=b1S, in_=b1d[:, :])
            cb1S = smr.tile([128, KD], F32, tag="cb1S", bufs=1)
            nc.sync.dma_start(out=cb1S, in_=cb1d[:, :])
            cb2S = smr.tile([128, KD], F32, tag="cb2S", bufs=1)
            nc.sync.dma_start(out=cb2S, in_=cb2d[:, :])
            # bf16 broadcast copies (B cols) for PSUM bias injection, + identity
            eye = wrec.tile([128, 128], BF, tag="eye")
            nc.sync.dma_start(out=eye, in_=eyed[:, :])
            b1b = smr.tile([128, KD, B], BF, tag="b1b", bufs=1)
            nc.vector.tensor_copy(b1b, bc(b1S[:, :], B, 2))
            cb1b = smr.tile([128, KD, B], BF, tag="cb1b", bufs=1)
            nc.vector.tensor_copy(cb1b, bc(cb1S[:, :], B, 2))
            cb2b = smr.tile([128, KD, B], BF, tag="cb2b", bufs=1)
            nc.vector.tensor_copy(cb2b, bc(cb2S[:, :], B, 2))
            outgS = smr.tile([128, KD], F32, tag="outgS", bufs=1)
            nc.sync.dma_start(out=outgS, in_=outgd[:, :])
            outbS = smr.tile([128, KD], F32, tag="outbS", bufs=1)
            nc.sync.dma_start(out=outbS, in_=outbd[:, :])
            Hc = [wrec.tile([128, KD, 128], BF, tag=f"H{mt}", name=f"H{mt}")
                  for mt in range(MT)]

            import os as _os
            T_RUN = int(_os.environ.get("T_TRUNC", T))
            if T_RUN < T:
                for Hcm in Hc:
                    nc.vector.memset(Hcm, 0.0)

            NBLK = (T_RUN + 7) // 8
            ucb = []

            def uc_fetch(mt):
                tile_ = wrec.tile([128, MH, 128], BF, tag="ucb", bufs=2,
                                  name=f"ucb{mt}")
                nc.sync.dma_start(out=tile_, in_=ucT[:, :, mt * 128:(mt + 1) * 128])
                ucb.append(tile_)

            uc_fetch(0)
            if KDEBUG:
                nc.sync.dma_start(out=dbg_ctx[:, :],
                                  in_=ctxF[:, :, :].rearrange("p k n -> p (k n)"))
                nc.sync.dma_start(out=dbg_uc[:, :],
                                  in_=ucb[0][:, :, :].rearrange("p m c -> p (m c)"))

            # lm chunk emission: during step t of block w, process chunks of
            # block w-1 (Hc[w-1] is complete); 13 chunks spread over 8 steps.
            LM_SPLIT = [0, 2, 4, 6, 8, 10, 11, 12, 13]

            def lm_chunks(mt, j0, j1):
                for j in range(j0, j1):
                    lw = wrec.tile([128, KD, 512], BF, tag="lmw", bufs=3)
                    nc.sync.dma_start(
                        out=lw,
                        in_=lmTd[:, j * 512:(j + 1) * 512].rearrange("(k p) n -> p k n",
                                                                     p=128))
                    ps = pou.tile([128, 512], F32, tag="plm", bufs=2)
                    for k in range(KD):
                        nc.tensor.matmul(ps, Hc[mt][:, k, :], lw[:, k, :],
                                         start=(k == 0), stop=(k == KD - 1))
                    ot = smr.tile([128, 512], F32, tag="otile", bufs=3)
                    if j % 2 == 0:
                        nc.scalar.activation(ot, ps, AF.Copy)
                    else:
                        nc.vector.tensor_copy(ot, ps)
                    nc.sync.dma_start(out=outd[mt * 128:(mt + 1) * 128,
                                               j * 512:(j + 1) * 512], in_=ot)

            for t in range(T_RUN):
                ctx_t = ctx_r[:, t]
                mtb, off = t // 8, (t % 8) * B
                if t % 8 == 0 and mtb + 1 < NBLK:
                    uc_fetch(mtb + 1)
                if t > 0:
                    hprev = Hc[(t - 1) // 8][:, :, ((t - 1) % 8) * B:((t - 1) % 8) * B + B]
                u = prc.tile([128, MH, B], F32, tag="pu")
                for m in range(MH):
                    nc.tensor.matmul(u[:, m, :], eye, ucb[mtb][:, m, off:off + B],
                                     start=True, stop=(t == 0))
                    if t > 0:
                        for k in range(KD):
                            nc.tensor.matmul(u[:, m, :], Whu[:, k, m * 128:(m + 1) * 128],
                                             hprev[:, k, :], start=False,
                                             stop=(k == KD - 1))
                hsS = smr.tile([128, MH, B], BF, tag="hsS")
                nc.scalar.activation(hsS, u, AF.Gelu)
                # tf branch (off critical path): w = tf - ctx
                w_t = smr.tile([128, KD, B], F32, tag="w_t")
                if t > 0:
                    tfp = prc.tile([128, KD, B], F32, tag="ptf")
                    for m in range(KD):
                        for k in range(KD):
                            nc.tensor.matmul(tfp[:, m, :], Wtf[:, k, m * 128:(m + 1) * 128],
                                             hprev[:, k, :], start=(k == 0),
                                             stop=(k == KD - 1))
                    nc.vector.tensor_sub(w_t, tfp, ctx_t)
                else:
                    nc.vector.tensor_scalar_mul(w_t, ctx_t, -1.0)
                cpr = pr6.tile([128, KD, B], F32, tag="p6")
                for m in range(KD):
                    for k in range(MH):
                        nc.tensor.matmul(cpr[:, m, :], V1w[:, k, m * 128:(m + 1) * 128],
                                         hsS[:, k, :], start=(k == 0), stop=False)
                    nc.tensor.matmul(cpr[:, m, :], eye, b1b[:, m, :],
                                     start=False, stop=True)
                # cpf stored b-major so per-b layernorm views are contiguous
                # (gpsimd firmware requires contiguous [128, F] rows)
                cpf = smr.tile([128, B, KD], F32, tag="cpf")
                nc.scalar.activation(cpf[:, :, :].rearrange("p b k -> p k b"),
                                     cpr, AF.Gelu)
                # l2norm via gpsimd rmsnorm (no act-table); 1/sqrt(D) folded
                # into cw1 host-side.
                corePool = smr.tile([128, B, KD], F32, tag="corePool")
                for b in range(B):
                    nc.gpsimd.layernorm(corePool[:, b, :], cpf[:, b, :],
                                        subtract_mean=False, eps=1e-10)
                coreS = smr.tile([128, KD, B], BF, tag="coreS")
                nc.vector.tensor_copy(coreS,
                                      corePool[:, :, :].rearrange("p b k -> p k b"))
                gm = pr6.tile([128, KD, B], F32, tag="p6")
                for m in range(KD):
                    for k in range(KD):
                        nc.tensor.matmul(gm[:, m, :], cw1[:, k, m * 128:(m + 1) * 128],
                                         coreS[:, k, :], start=(k == 0), stop=False)
                    nc.tensor.matmul(gm[:, m, :], eye, cb1b[:, m, :],
                                     start=False, stop=True)
                gmS = smr.tile([128, KD, B], BF, tag="gmS")
                nc.scalar.activation(gmS, gm, AF.Gelu)
                cfp = pr6.tile([128, KD, B], F32, tag="p6")
                for m in range(KD):
                    for k in range(KD):
                        nc.tensor.matmul(cfp[:, m, :], cw2[:, k, m * 128:(m + 1) * 128],
                                         gmS[:, k, :], start=(k == 0), stop=False)
                    nc.tensor.matmul(cfp[:, m, :], eye, cb2b[:, m, :],
                                     start=False, stop=True)
                cfF = smr.tile([128, KD, B], F32, tag="cfF")
                nc.scalar.activation(cfF, cfp, AF.Copy)
                # gate cf-branch folded through cp_w2: reads g1 (gmS) directly,
                # in parallel with the cfp/cfF branch
                gp = pr6.tile([128, KD, B], F32, tag="pgp")
                for m in range(KD):
                    for k in range(KD):
                        nc.tensor.matmul(gp[:, m, :], gw2[:, k, m * 128:(m + 1) * 128],
                                         gmS[:, k, :], start=(k == 0), stop=False)
                    nc.tensor.matmul(gp[:, m, :], eye, gctx_r[:, t, m, :],
                                     start=False, stop=True)
                # gate via tanh (shares the gelu act table):
                #   sigmoid(x) = 0.5*(1 + tanh(x/2))
                #   hp = gate*(cf+tf-ctx) + ctx = 0.5*(tanh+1)*(cf+tf-ctx) + ctx
                tG = smr.tile([128, KD, B], F32, tag="gateS")
                nc.scalar.activation(tG, gp, AF.Tanh, scale=0.5)
                a1 = smr.tile([128, KD, B], F32, tag="a1")
                nc.vector.tensor_add(a1, cfF, w_t)
                q1 = smr.tile([128, KD, B], F32, tag="q1")
                nc.vector.scalar_tensor_tensor(q1, tG, 1.0, a1, OP.add, OP.mult)
                hp = smr.tile([128, B, KD], F32, tag="hp")
                nc.vector.scalar_tensor_tensor(
                    hp[:, :, :].rearrange("p b k -> p k b"),
                    q1, 0.5, ctx_t, OP.mult, OP.add)
                # LN + affine via gpsimd layernorm (no act-table), then clip.
                lnout = smr.tile([128, B, KD], F32, tag="lnout")
                for b in range(B):
                    nc.gpsimd.layernorm(lnout[:, b, :], hp[:, b, :],
                                        gamma_ap=outgS[:, :], beta_ap=outbS[:, :],
                                        subtract_mean=True, eps=1e-5)
                nc.vector.tensor_scalar(Hc[t // 8][:, :, (t % 8) * B:(t % 8) * B + B],
                                        lnout[:, :, :].rearrange("p b k -> p k b"),
                                        5.0, -5.0, OP.min, OP.max)
                if KDEBUG and t == min(7, T_RUN - 1):
                    nc.sync.dma_start(out=dbg_h[:, :],
                                      in_=Hc[0][:, :, :].rearrange("p k c -> p (k c)"))
                if KDEBUG and t == int(_os.environ.get("KPROBE_T", 0)):
                    for nm, tile_ in [("hs", hsS), ("cpf", cpf), ("core", coreS),
                                      ("gm", gmS), ("cf", cfF), ("tg", tG),
                                      ("hp", hp), ("ln", lnout)]:
                        nc.sync.dma_start(
                            out=dbg_st[nm][:, :],
                            in_=tile_[:, :, :].rearrange("p a b -> p (a b)"))
                # interleave lm-head work for the previous (complete) block
                if t >= 8 and T_RUN == T:
                    lm_chunks(mtb - 1, LM_SPLIT[t % 8], LM_SPLIT[t % 8 + 1])

            # lm-head tail: last block (or all blocks on truncated debug runs)
            tail_blocks = range(NBLK - 1, NBLK) if T_RUN == T else range(NBLK)
            for mt in tail_blocks:
                lm_chunks(mt, 0, NCH)

    nc.finalize()
    return nc


_NC_CACHE = {}


def _get_nc():
    if "nc" not in _NC_CACHE:
        _NC_CACHE["nc"] = build_nc()
    return _NC_CACHE["nc"]


def _prep_inputs(inputs):
    f = lambda x: np.asarray(x, np.float32)
    tok = np.asarray(inputs["token_ids"]).astype(np.int64)
    emb, pos = f(inputs["emb"]), f(inputs["pos_emb"])
    x0 = emb[tok.reshape(-1)] + np.tile(pos[:T], (B, 1))
    com = {"x0T": x0.T.astype(bfnp)}
    aiw, aib = f(inputs["attn_in_w"]), f(inputs["attn_in_b"])
    aow, aob = f(inputs["attn_out_w"]), f(inputs["attn_out_b"])
    for l in range(NL):
        wqk = np.zeros((D, 2048), np.float32)
        bqk = np.zeros(2048, np.float32)
        wv = np.zeros((D, 1024), np.float32)
        bv = np.zeros(1024, np.float32)
        wao = np.zeros((1024, D), np.float32)
        for h in range(NH):
            wqk[:, h * 128:h * 128 + HD] = aiw[l, h * HD:(h + 1) * HD, :].T
            wqk[:, 1024 + h * 128:1024 + h * 128 + HD] = aiw[l, D + h * HD:D + (h + 1) * HD, :].T
            bqk[h * 128:h * 128 + HD] = aib[l, h * HD:(h + 1) * HD]
            bqk[1024 + h * 128:1024 + h * 128 + HD] = aib[l, D + h * HD:D + (h + 1) * HD]
            wv[:, h * 128:h * 128 + HD] = aiw[l, 2 * D + h * HD:2 * D + (h + 1) * HD, :].T
            bv[h * 128:h * 128 + HD] = aib[l, 2 * D + h * HD:2 * D + (h + 1) * HD]
            wao[h * 128:h * 128 + HD, :] = aow[l][:, h * HD:(h + 1) * HD].T
        com[f"wqk{l}"] = wqk.astype(bfnp)
        com[f"bqk{l}"] = bqk.reshape(16, 128).T.copy()
        com[f"wv{l}"] = wv.astype(bfnp)
        com[f"bv{l}"] = bv.reshape(1, 1024).astype(bfnp)
        com[f"wao{l}"] = wao.astype(bfnp)
        com[f"bao{l}"] = aob[l].reshape(KD, 128).T.copy()
        com[f"ff1T{l}"] = f(inputs["ff_w1"])[l].T.astype(bfnp).copy()
        com[f"bff1{l}"] = f(inputs["ff_b1"])[l].reshape(MF, 128).T.copy()
        com[f"ff2T{l}"] = f(inputs["ff_w2"])[l].T.astype(bfnp).copy()
        com[f"bff2{l}"] = f(inputs["ff_b2"])[l].reshape(KD, 128).T.copy()
        com[f"g1{l}"] = f(inputs["n1_g"])[l].reshape(KD, 128).T.copy()
        com[f"bb1{l}"] = f(inputs["n1_b"])[l].reshape(KD, 128).T.copy()
        com[f"g2{l}"] = f(inputs["n2_g"])[l].reshape(KD, 128).T.copy()
        com[f"bb2{l}"] = f(inputs["n2_b"])[l].reshape(KD, 128).T.copy()
    com["encg"] = f(inputs["enc_norm_g"]).reshape(KD, 128).T.copy()
    com["encb"] = f(inputs["enc_norm_b"]).reshape(KD, 128).T.copy()
    tk, tq = np.meshgrid(np.arange(64), np.arange(64), indexing="ij")
    com["maskT"] = (tk <= tq).astype(bfnp)
    com["eyed"] = np.eye(128, dtype=bfnp)
    com["V0d"] = f(inputs["V0"]).astype(bfnp)
    com["V1d"] = f(inputs["V1"]).astype(bfnp)
    R, tw = f(inputs["R"]), f(inputs["temp_w"])
    # folded recurrent weights: u += (alpha*R@V0)^T h ; tf = (alpha*R@tw^T)^T h
    com["Whud"] = (ALPHA * R @ f(inputs["V0"])).astype(bfnp)
    com["Wtfd"] = (ALPHA * R @ tw.T).astype(bfnp)
    # rmsnorm(x) = sqrt(D) * l2norm(x): fold the 1/sqrt(D) into cp_w1
    com["cw1d"] = (f(inputs["cp_w1"]).T / np.sqrt(D)).astype(bfnp).copy()
    com["cw2d"] = f(inputs["cp_w2"]).T.astype(bfnp).copy()
    gw = f(inputs["gate_w"])
    com["gw1d"] = gw[:, :D].T.astype(bfnp).copy()
    # gate cf-branch folded through cp_w2: gs_cf = Wgg^T g1 (+ gwB@cp_b2 -> gb)
    gwB = gw[:, D:]
    com["gw2d"] = np.ascontiguousarray((gwB @ f(inputs["cp_w2"])).T).astype(bfnp)
    com["gbd"] = (f(inputs["gate_b"]) + gwB @ f(inputs["cp_b2"])).reshape(KD, 128).T.copy()
    com["b0d"] = f(inputs["b0"]).reshape(MH, 128).T.copy()
    com["b1d"] = f(inputs["b1"]).reshape(KD, 128).T.copy()
    com["cb1d"] = f(inputs["cp_b1"]).reshape(KD, 128).T.copy()
    com["cb2d"] = f(inputs["cp_b2"]).reshape(KD, 128).T.copy()
    com["outgd"] = np.ascontiguousarray(f(inputs["out_g"]).reshape(KD, 128).T)
    com["outbd"] = np.ascontiguousarray(f(inputs["out_b"]).reshape(KD, 128).T)
    lmp = np.zeros((VP, D), np.float32)
    lmp[:V] = f(inputs["lm_head"])
    lmT = lmp.T.astype(bfnp)
    shards = [np.ascontiguousarray(lmT[:, c * VS:(c + 1) * VS]) for c in range(8)]
    return com, shards


LAST_RESULT = {}


def kernel(**inputs):
    import os
    nc = _get_nc()
    com, shards = _prep_inputs(inputs)
    in_maps = [{**com, "lmTd": shards[c]} for c in range(8)]
    kw = {}
    if os.environ.get("KTRACE"):
        kw = dict(trace=True, tmpdir=os.environ.get("KTRACE_DIR", "/root/problem/trace_out"))
    res = run_bass_kernel_spmd(nc, in_maps, core_ids=list(range(8)), **kw)
    LAST_RESULT["res"] = res
    parts = [res.results[c]["out"] for c in range(8)]          # each [1024, VS], t-major rows
    full = np.concatenate(parts, axis=1)[:, :V]                # [1024, V]
    return np.ascontiguousarray(full.reshape(T, B, V).transpose(1, 0, 2))



# revision 51
# speedup vs baseline: 1.3730x; 1.0135x over previous
"""Trainium2 Bass kernel for nn_AgnisV5: 2-layer GPT encoder + gated
hierarchical recurrence + lm_head, SPMD over 8 NeuronCores.

Strategy: encoder + recurrence replicated on all cores (no collectives);
lm_head vocab-sharded 8 ways. Forward-pass simplification: stop_gradient is
identity, so the "settled" hierarchy path equals the differentiable shadow
path and blend == core_sh.

Layouts: feature-major activations [128p, K, N] (feature f = k*128 + p).
Encoder token columns are b-major (n = b*64 + t); recurrence/H/lm_head use
t-major (n = t*16 + b) via strided views of ctx.
"""
import numpy as np
import ml_dtypes
from contextlib import ExitStack

import concourse.bass as bass
import concourse.mybir as mybir
import concourse.tile as tile
from concourse import bacc
from concourse.bass_utils import run_bass_kernel_spmd

AF = mybir.ActivationFunctionType
OP = mybir.AluOpType
BF = mybir.dt.float16
F32 = mybir.dt.float32

D, HID, FFN, NH, NL, T, V, B = 768, 3072, 2048, 8, 2, 64, 50257, 16
HD = 96
KD = D // 128          # 6
MH = HID // 128        # 24
MF = FFN // 128        # 16
NTOK = B * T           # 1024
MT = NTOK // 128       # 8
VP = 53248             # padded vocab = 8*6656
VS = VP // 8           # 6656 per core
NCH = VS // 512        # 13
ALPHA = 0.4
QK_SCALE = 1.0 / np.sqrt(96.0)

bfnp = np.float16


_FLAGS = {"ln_triv": False, "bv0": False}


def bc(ap, reps, pos):
    """Insert a stride-0 (broadcast) free dim at position pos (1-based over free dims)."""
    newap = list(ap.ap)
    newap.insert(pos, [0, reps])
    return bass.AP(tensor=ap.tensor, offset=ap.offset, ap=newap)


def build_nc():
    LN_TRIV = _FLAGS["ln_triv"]
    BV0 = _FLAGS["bv0"]
    nc = bacc.Bacc(None, target_bir_lowering=False)

    # ---- DRAM I/O ----
    x0T = nc.dram_tensor("x0T", [D, NTOK], BF, kind="ExternalInput")
    enc_in = {}
    for l in range(NL):
        enc_in[f"wqk{l}"] = nc.dram_tensor(f"wqk{l}", [D, 2048], BF, kind="ExternalInput")
        enc_in[f"wv{l}"] = nc.dram_tensor(f"wv{l}", [D, 1024], BF, kind="ExternalInput")
        enc_in[f"wao{l}"] = nc.dram_tensor(f"wao{l}", [1024, D], BF, kind="ExternalInput")
        enc_in[f"ff1T{l}"] = nc.dram_tensor(f"ff1T{l}", [D, FFN], BF, kind="ExternalInput")
        enc_in[f"ff2T{l}"] = nc.dram_tensor(f"ff2T{l}", [FFN, D], BF, kind="ExternalInput")
        enc_in[f"bqk{l}"] = nc.dram_tensor(f"bqk{l}", [128, 16], F32, kind="ExternalInput")
        enc_in[f"bv{l}"] = nc.dram_tensor(f"bv{l}", [1, 1024], BF, kind="ExternalInput")
        enc_in[f"bao{l}"] = nc.dram_tensor(f"bao{l}", [128, KD], F32, kind="ExternalInput")
        enc_in[f"bff1{l}"] = nc.dram_tensor(f"bff1{l}", [128, MF], F32, kind="ExternalInput")
        enc_in[f"bff2{l}"] = nc.dram_tensor(f"bff2{l}", [128, KD], F32, kind="ExternalInput")
        enc_in[f"g1{l}"] = nc.dram_tensor(f"g1{l}", [128, KD], F32, kind="ExternalInput")
        enc_in[f"bb1{l}"] = nc.dram_tensor(f"bb1{l}", [128, KD], F32, kind="ExternalInput")
        enc_in[f"g2{l}"] = nc.dram_tensor(f"g2{l}", [128, KD], F32, kind="ExternalInput")
        enc_in[f"bb2{l}"] = nc.dram_tensor(f"bb2{l}", [128, KD], F32, kind="ExternalInput")
    encg = nc.dram_tensor("encg", [128, KD], F32, kind="ExternalInput")
    encb = nc.dram_tensor("encb", [128, KD], F32, kind="ExternalInput")
    maskT = nc.dram_tensor("maskT", [64, 64], BF, kind="ExternalInput")
    V0d = nc.dram_tensor("V0d", [D, HID], BF, kind="ExternalInput")
    V1d = nc.dram_tensor("V1d", [HID, D], BF, kind="ExternalInput")
    Whud = nc.dram_tensor("Whud", [D, HID], BF, kind="ExternalInput")
    Wtfd = nc.dram_tensor("Wtfd", [D, D], BF, kind="ExternalInput")
    cw1d = nc.dram_tensor("cw1d", [D, D], BF, kind="ExternalInput")
    cw2d = nc.dram_tensor("cw2d", [D, D], BF, kind="ExternalInput")
    gw1d = nc.dram_tensor("gw1d", [D, D], BF, kind="ExternalInput")
    gw2d = nc.dram_tensor("gw2d", [D, D], BF, kind="ExternalInput")
    b0d = nc.dram_tensor("b0d", [128, MH], F32, kind="ExternalInput")
    b1d = nc.dram_tensor("b1d", [128, KD], F32, kind="ExternalInput")
    cb1d = nc.dram_tensor("cb1d", [128, KD], F32, kind="ExternalInput")
    cb2d = nc.dram_tensor("cb2d", [128, KD], F32, kind="ExternalInput")
    gbd = nc.dram_tensor("gbd", [128, KD], F32, kind="ExternalInput")
    outgd = nc.dram_tensor("outgd", [128, KD], F32, kind="ExternalInput")
    outbd = nc.dram_tensor("outbd", [128, KD], F32, kind="ExternalInput")
    lmTd = nc.dram_tensor("lmTd", [D, VS], BF, kind="ExternalInput")
    eyed = nc.dram_tensor("eyed", [128, 128], BF, kind="ExternalInput")
    outd = nc.dram_tensor("out", [NTOK, VS], F32, kind="ExternalOutput")
    import os as _os0
    KDEBUG = bool(_os0.environ.get("KDEBUG"))
    if KDEBUG:
        dbg_ctx = nc.dram_tensor("dbg_ctx", [128, KD * NTOK], BF, kind="ExternalOutput")
        dbg_uc = nc.dram_tensor("dbg_uc", [128, MH * 128], BF, kind="ExternalOutput")
        dbg_h = nc.dram_tensor("dbg_h", [128, KD * 128], BF, kind="ExternalOutput")
        dbg_st = {}
        for nm, width, dt_ in [("hs", MH * B, BF), ("cpf", KD * B, F32),
                               ("core", KD * B, BF), ("gm", KD * B, BF),
                               ("cf", KD * B, F32), ("tg", KD * B, F32),
                               ("hp", KD * B, F32), ("ln", KD * B, F32)]:
            dbg_st[nm] = nc.dram_tensor(f"dbg_{nm}", [128, width], dt_,
                                        kind="ExternalOutput")

    def r3(h, p=128):
        return h[:, :].rearrange("(k p) n -> p k n", p=p)

    with ExitStack() as ctx:
        tc = ctx.enter_context(tile.TileContext(nc))
        cross = ctx.enter_context(tc.tile_pool(name="cross", bufs=1))

        # constants (cross-scope)
        ones_col = cross.tile([128, 1], BF, tag="ones_col")
        nc.vector.memset(ones_col, 1.0)
        ones64 = cross.tile([64, 1], BF, tag="ones64")
        nc.vector.memset(ones64, 1.0)
        ones_row_f = cross.tile([1, 128], F32, tag="ones_row_f")
        nc.vector.memset(ones_row_f, 1.0)
        ones_r64f = cross.tile([1, 64], F32, tag="ones_r64f")
        nc.vector.memset(ones_r64f, 1.0)
        mask_sb = cross.tile([64, 64], BF, tag="mask")
        nc.sync.dma_start(out=mask_sb, in_=maskT[:, :])
        eye = cross.tile([128, 128], BF, tag="eye")
        nc.sync.dma_start(out=eye, in_=eyed[:, :])
        eps5 = cross.tile([128, 1], F32, tag="eps5")
        nc.vector.memset(eps5, 1e-5)
        eps24 = cross.tile([128, 1], F32, tag="eps24")
        nc.vector.memset(eps24, 1e-24)
        ctxF = cross.tile([128, KD, NTOK], BF, tag="ctxF")
        gctx = cross.tile([128, KD, NTOK], BF, tag="gctx")
        # DRAM scratch for uc (tile pool so RAW deps across DMAs are tracked)
        ucdram = ctx.enter_context(tc.tile_pool(name="ucdram", bufs=1, space="DRAM"))
        ucT = ucdram.tile([128, MH, NTOK], BF, tag="ucT")

        # ================= ENCODER SCOPE =================
        with tc.tile_pool(name="wenc", bufs=1) as wenc, \
             tc.tile_pool(name="bige", bufs=1) as bige, \
             tc.tile_pool(name="sme", bufs=1) as sme, \
             tc.tile_pool(name="pmm", bufs=2, space="PSUM") as pmm, \
             tc.tile_pool(name="pst", bufs=1, space="PSUM") as pst, \
             tc.tile_pool(name="pbc", bufs=2, space="PSUM") as pbc:

            def ln_fm(x_sb, g_ap, b_ap, out, l2_out=None, triv=False):
                """x_sb [128, KD, 1024] bf16 -> LN -> out; optional l2 -> l2_out.
                triv: gamma==1 and beta==0 (runtime-specialized)."""
                for j in range(2):
                    jj = slice(j * 512, (j + 1) * 512)
                    s12 = pst.tile([1, 2, 512], F32, tag="s1")
                    for k in range(KD):
                        sq = bige.tile([128, 512], BF, tag="sqc", bufs=2)
                        nc.vector.tensor_mul(sq, x_sb[:, k, jj], x_sb[:, k, jj])
                        nc.tensor.matmul(s12[:, 0, :], ones_col, x_sb[:, k, jj],
                                         start=(k == 0), stop=(k == KD - 1))
                        nc.tensor.matmul(s12[:, 1, :], ones_col, sq,
                                         start=(k == 0), stop=(k == KD - 1))
                    m = sme.tile([1, 512], F32, tag="mtag", bufs=2)
                    nc.vector.tensor_scalar_mul(m, s12[:, 0, :], 1.0 / D)
                    msq = sme.tile([1, 512], F32, tag="scr512", bufs=2)
                    nc.vector.tensor_mul(msq, m, m)
                    var = sme.tile([1, 512], F32, tag="scr512", bufs=2)
                    nc.vector.scalar_tensor_tensor(var, s12[:, 1, :], 1.0 / D,
                                                   msq, OP.mult, OP.subtract)
                    sd = sme.tile([1, 512], F32, tag="scr512", bufs=2)
                    nc.scalar.activation(sd, var, AF.Sqrt, bias=eps5[:1, :])
                    ac = sme.tile([1, 2, 512], F32, tag="acf")
                    nc.vector.reciprocal(ac[:, 0, :], sd)
                    nc.vector.scalar_tensor_tensor(ac[:, 1, :], m, -1.0,
                                                   ac[:, 0, :], OP.mult, OP.mult)
                    pac = pbc.tile([128, 2, 512], F32, tag="pabc", bufs=1)
                    for a in range(2):
                        nc.tensor.matmul(pac[:, a, :], ones_row_f, ac[:, a, :],
                                         start=True, stop=True)
                    pab = bige.tile([128, 2, 512], BF, tag="pab16", bufs=2)
                    nc.scalar.activation(pab, pac, AF.Copy)
                    for k in range(KD):
                        if triv:
                            t1 = bige.tile([128, 512], BF, tag="t1", bufs=2)
                            nc.vector.tensor_mul(t1, x_sb[:, k, jj], pab[:, 0, :])
                            nc.vector.tensor_add(out[:, k, jj], t1, pab[:, 1, :])
                        else:
                            t1 = bige.tile([128, 512], F32, tag="t1f", bufs=2)
                            nc.vector.tensor_mul(t1, x_sb[:, k, jj], pab[:, 0, :])
                            nc.vector.tensor_add(t1, t1, pab[:, 1, :])
                            nc.vector.tensor_scalar(out[:, k, jj], t1,
                                                    g_ap[:, k:k + 1],
                                                    b_ap[:, k:k + 1],
                                                    OP.mult, OP.add)
                if l2_out is None:
                    return
                for j in range(2):
                    jj = slice(j * 512, (j + 1) * 512)
                    s1t = pst.tile([1, 2, 512], F32, tag="s1")
                    s1 = s1t[:, 0, :]
                    for k in range(KD):
                        sq = bige.tile([128, 512], BF, tag="sqc", bufs=2)
                        nc.vector.tensor_mul(sq, out[:, k, jj], out[:, k, jj])
                        nc.tensor.matmul(s1, ones_col, sq,
                                         start=(k == 0), stop=(k == KD - 1))
                    sd = sme.tile([1, 512], F32, tag="scr512", bufs=2)
                    nc.scalar.activation(sd, s1, AF.Sqrt, bias=eps24[:1, :])
                    rr = sme.tile([1, 512], F32, tag="scr512", bufs=2)
                    nc.vector.reciprocal(rr, sd)
                    pat = pbc.tile([128, 2, 512], F32, tag="pabc", bufs=1)
                    pa = pat[:, 0, :]
                    nc.tensor.matmul(pa, ones_row_f, rr, start=True, stop=True)
                    pa16 = bige.tile([128, 512], BF, tag="pa16", bufs=2)
                    nc.scalar.activation(pa16, pa, AF.Copy)
                    for k in range(KD):
                        nc.vector.tensor_mul(l2_out[:, k, jj], out[:, k, jj], pa16)

            X = bige.tile([128, KD, NTOK], BF, tag="X")
            nc.sync.dma_start(out=X, in_=r3(x0T))
            gA = sme.tile([128, NL, KD], F32, tag="gA")
            bA = sme.tile([128, NL, KD], F32, tag="bA")
            gB = sme.tile([128, NL, KD], F32, tag="gB")
            bB = sme.tile([128, NL, KD], F32, tag="bB")
            for l in range(NL):
                nc.sync.dma_start(out=gA[:, l, :], in_=enc_in[f"g1{l}"][:, :])
                nc.sync.dma_start(out=bA[:, l, :], in_=enc_in[f"bb1{l}"][:, :])
                nc.sync.dma_start(out=gB[:, l, :], in_=enc_in[f"g2{l}"][:, :])
                nc.sync.dma_start(out=bB[:, l, :], in_=enc_in[f"bb2{l}"][:, :])

            for l in range(NL):
                XN = bige.tile([128, KD, NTOK], BF, tag="XN")
                ln_fm(X, gA[:, l, :], bA[:, l, :], XN, triv=LN_TRIV)
                wqk = wenc.tile([128, KD, 2048], BF, tag="wbig")
                nc.sync.dma_start(out=wqk, in_=r3(enc_in[f"wqk{l}"]))
                bqk = sme.tile([128, 16], F32, tag="bqk")
                nc.sync.dma_start(out=bqk, in_=enc_in[f"bqk{l}"][:, :])
                # v projection (token-major)
                wv = wenc.tile([128, KD, 1024], BF, tag="wsmall")
                nc.sync.dma_start(out=wv, in_=r3(enc_in[f"wv{l}"]))
                if not BV0:
                    ones_row = sme.tile([1, 128], BF, tag="ones_row")
                    nc.vector.memset(ones_row, 1.0)
                    bvr = sme.tile([1, 1024], BF, tag="bvr")
                    nc.sync.dma_start(out=bvr, in_=enc_in[f"bv{l}"][:, :])
                    bvs = bige.tile([128, 1024], BF, tag="bvs")
                    for j in range(2):
                        pb = pbc.tile([128, 2, 512], F32, tag="pabc", bufs=1)
                        nc.tensor.matmul(pb[:, 0, :], ones_row,
                                         bvr[:, j * 512:(j + 1) * 512],
                                         start=True, stop=True)
                        nc.vector.tensor_copy(bvs[:, j * 512:(j + 1) * 512],
                                              pb[:, 0, :])
                vS = bige.tile([64, B, 1024], BF, tag="vS")
                for b in range(B):
                    ps = pmm.tile([64, 1024], F32, tag="pmm")
                    for k in range(KD):
                        for j in range(2):
                            nc.tensor.matmul(ps[:, j * 512:(j + 1) * 512],
                                             XN[:, k, b * 64:(b + 1) * 64],
                                             wv[:, k, j * 512:(j + 1) * 512],
                                             start=(k == 0), stop=(k == KD - 1))
                    if not BV0:
                        nc.vector.tensor_add(vS[:, b, :], ps, bvs[:64, :])
                    elif b % 2 == 0:
                        nc.scalar.activation(vS[:, b, :], ps, AF.Copy)
                    else:
                        nc.vector.tensor_copy(vS[:, b, :], ps)
                # attention per head; additive mask injected into score PSUM
                ofS = bige.tile([128, NH, NTOK], BF, tag="ofS")
                for h in range(NH):
                    qh = bige.tile([128, NTOK], BF, tag="qh", bufs=1)
                    ph = pmm.tile([128, 1024], F32, tag="pmm")
                    for k in range(KD):
                        for j in range(2):
                            nc.tensor.matmul(ph[:, j * 512:(j + 1) * 512],
                                             wqk[:, k, h * 128:(h + 1) * 128],
                                             XN[:, k, j * 512:(j + 1) * 512],
                                             start=(k == 0), stop=(k == KD - 1))
                    nc.vector.tensor_scalar_add(qh, ph, bqk[:, h:h + 1])
                    kh = bige.tile([128, NTOK], BF, tag="kh", bufs=1)
                    ph2 = pmm.tile([128, 1024], F32, tag="pmm")
                    for k in range(KD):
                        for j in range(2):
                            nc.tensor.matmul(ph2[:, j * 512:(j + 1) * 512],
                                             wqk[:, k, 1024 + h * 128:1024 + (h + 1) * 128],
                                             XN[:, k, j * 512:(j + 1) * 512],
                                             start=(k == 0), stop=(k == KD - 1))
                    nc.scalar.activation(kh, ph2, AF.Identity,
                                         bias=bqk[:, 8 + h:9 + h])
                    sT = pmm.tile([64, 1024], F32, tag="pmm")
                    for b in range(B):
                        nc.tensor.matmul(sT[:, b * 64:(b + 1) * 64],
                                         kh[:, b * 64:(b + 1) * 64],
                                         qh[:, b * 64:(b + 1) * 64],
                                         start=True, stop=False)
                        nc.tensor.matmul(sT[:, b * 64:(b + 1) * 64],
                                         eye[:64, :64], mask_sb,
                                         start=False, stop=True)
                    eT = bige.tile([64, NTOK], BF, tag="eT", bufs=2)
                    nc.scalar.activation(eT, sT, AF.Exp, scale=QK_SCALE)
                    ssum = pst.tile([1, 2, 512], F32, tag="s1")
                    for a in range(2):
                        nc.tensor.matmul(ssum[:, a, :], ones64,
                                         eT[:, a * 512:(a + 1) * 512],
                                         start=True, stop=True)
                    rs2 = sme.tile([1, 2, 512], F32, tag="acf")
                    rs = rs2[:, :, :].rearrange("p a c -> p (a c)")
                    nc.vector.reciprocal(rs, ssum[:, :, :].rearrange("p a c -> p (a c)"))
                    rb64 = pmm.tile([128, 1024], F32, tag="pmm")
                    for a in range(2):
                        nc.tensor.matmul(rb64[:64, a * 512:(a + 1) * 512],
                                         ones_r64f, rs[:, a * 512:(a + 1) * 512],
                                         start=True, stop=True)
                    eN = bige.tile([64, NTOK], BF, tag="eN", bufs=2)
                    nc.vector.tensor_mul(eN, eT, rb64[:64, :])
                    oh = pmm.tile([128, 1024], F32, tag="pmm")
                    for b in range(B):
                        nc.tensor.matmul(oh[:, b * 64:(b + 1) * 64],
                                         vS[:, b, h * 128:(h + 1) * 128],
                                         eN[:, b * 64:(b + 1) * 64],
                                         start=True, stop=True)
                    if h % 2 == 0:
                        nc.scalar.activation(ofS[:, h, :], oh, AF.Copy)
                    else:
                        nc.vector.tensor_copy(ofS[:, h, :], oh)
                # attn out + residual (in place on X)
                wao = wenc.tile([128, NH, D], BF, tag="wsmall")
                nc.sync.dma_start(out=wao,
                                  in_=enc_in[f"wao{l}"][:, :].rearrange("(k p) n -> p k n", p=128))
                bao = sme.tile([128, KD], F32, tag="bao")
                nc.sync.dma_start(out=bao, in_=enc_in[f"bao{l}"][:, :])
                for mo in range(KD):
                    ps = pmm.tile([128, 1024], F32, tag="pmm")
                    for kv in range(NH):
                        for j in range(2):
                            nc.tensor.matmul(ps[:, j * 512:(j + 1) * 512],
                                             wao[:, kv, mo * 128:(mo + 1) * 128],
                                             ofS[:, kv, j * 512:(j + 1) * 512],
                                             start=(kv == 0), stop=(kv == NH - 1))
                    t2 = bige.tile([128, 1024], F32, tag="t2", bufs=2)
                    nc.vector.tensor_scalar_add(t2, ps, bao[:, mo:mo + 1])
                    nc.vector.tensor_add(X[:, mo, :], X[:, mo, :], t2)
                # ffn
                XN2 = bige.tile([128, KD, NTOK], BF, tag="XN")
                ln_fm(X, gB[:, l, :], bB[:, l, :], XN2, triv=LN_TRIV)
                w1 = wenc.tile([128, KD, FFN], BF, tag="wbig")
                nc.sync.dma_start(out=w1, in_=r3(enc_in[f"ff1T{l}"]))
                bf1 = sme.tile([128, MF], F32, tag="bf1")
                nc.sync.dma_start(out=bf1, in_=enc_in[f"bff1{l}"][:, :])
                G = bige.tile([128, MF, NTOK], BF, tag="G")
                for mf in range(MF):
                    ps = pmm.tile([128, 1024], F32, tag="pmm")
                    for k in range(KD):
                        for j in range(2):
                            nc.tensor.matmul(ps[:, j * 512:(j + 1) * 512],
                                             w1[:, k, mf * 128:(mf + 1) * 128],
                                             XN2[:, k, j * 512:(j + 1) * 512],
                                             start=(k == 0), stop=(k == KD - 1))
                    nc.scalar.activation(G[:, mf, :], ps, AF.Gelu, bias=bf1[:, mf:mf + 1])
                w2 = wenc.tile([128, MF, D], BF, tag="wbig")
                nc.sync.dma_start(out=w2, in_=r3(enc_in[f"ff2T{l}"]))
                bf2 = sme.tile([128, KD], F32, tag="bao")
                nc.sync.dma_start(out=bf2, in_=enc_in[f"bff2{l}"][:, :])
                for mo in range(KD):
                    ps = pmm.tile([128, 1024], F32, tag="pmm")
                    for kf in range(MF):
                        for j in range(2):
                            nc.tensor.matmul(ps[:, j * 512:(j + 1) * 512],
                                             w2[:, kf, mo * 128:(mo + 1) * 128],
                                             G[:, kf, j * 512:(j + 1) * 512],
                                             start=(kf == 0), stop=(kf == MF - 1))
                    t2 = bige.tile([128, 1024], F32, tag="t2", bufs=2)
                    nc.vector.tensor_scalar_add(t2, ps, bf2[:, mo:mo + 1])
                    nc.vector.tensor_add(X[:, mo, :], X[:, mo, :], t2)

            # final norm + l2 -> ctxF (cross pool)
            eg = sme.tile([128, KD], F32, tag="eg")
            eb = sme.tile([128, KD], F32, tag="eb")
            nc.sync.dma_start(out=eg, in_=encg[:, :])
            nc.sync.dma_start(out=eb, in_=encb[:, :])
            pre = bige.tile([128, KD, NTOK], BF, tag="XN")
            ln_fm(X, eg, eb, pre, l2_out=ctxF, triv=LN_TRIV)

            # gate-ctx precompute -> gctx (cross pool)
            gw1 = wenc.tile([128, KD, D], BF, tag="wsmall")
            nc.sync.dma_start(out=gw1, in_=r3(gw1d))
            gbS = sme.tile([128, KD], F32, tag="gbS")
            nc.sync.dma_start(out=gbS, in_=gbd[:, :])
            for mo in range(KD):
                ps = pmm.tile([128, 1024], F32, tag="pmm")
                for k in range(KD):
                    for j in range(2):
                        nc.tensor.matmul(ps[:, j * 512:(j + 1) * 512],
                                         gw1[:, k, mo * 128:(mo + 1) * 128],
                                         ctxF[:, k, j * 512:(j + 1) * 512],
                                         start=(k == 0), stop=(k == KD - 1))
                nc.vector.tensor_scalar_add(gctx[:, mo, :], ps, gbS[:, mo:mo + 1])

        # ===== uc precompute: uc[:, t*16+b] = V0^T ctx[:, b*64+t] + b0 =====
        ctx_tb = ctxF[:, :, :].rearrange("p k (b t) -> p k t b", b=B)
        with tc.tile_pool(name="wuc", bufs=1) as wuc, \
             tc.tile_pool(name="sucs", bufs=2) as sucs, \
             tc.tile_pool(name="puc", bufs=1, space="PSUM") as puc:
            V0e = wuc.tile([128, KD, HID], BF, tag="V0e")
            nc.sync.dma_start(out=V0e, in_=r3(V0d))
            b0rowE = sucs.tile([1, MH, 128], BF, tag="b0rowE", bufs=1)
            _b0ap = b0d[:, :]
            nc.gpsimd.dma_start(out=b0rowE, in_=bass.AP(tensor=_b0ap.tensor, offset=0,
                                                        ap=[[0, 1], [1, MH], [MH, 128]]))
            ones128r = sucs.tile([1, 128], BF, tag="ones128r", bufs=1)
            nc.vector.memset(ones128r, 1.0)
            for mt in range(MT):
                pu8 = puc.tile([128, MH, 128], F32, tag="puc")
                for m in range(MH):
                    for k in range(KD):
                        nc.tensor.matmul(pu8[:, m, :], V0e[:, k, m * 128:(m + 1) * 128],
                                         ctx_tb[:, k, mt * 8:(mt + 1) * 8, :],
                                         start=(k == 0), stop=False)
                    nc.tensor.matmul(pu8[:, m, :], b0rowE[:, m, :], ones128r,
                                     start=False, stop=True)
                ucs = sucs.tile([128, MH, 128], BF, tag="ucs")
                nc.scalar.activation(ucs, pu8, AF.Copy)
                nc.sync.dma_start(out=ucT[:, :, mt * 128:(mt + 1) * 128], in_=ucs)

        # ================= RECURRENCE SCOPE =================
        ctx_r = ctxF[:, :, :].rearrange("p k (b t) -> p t k b", b=B)
        gctx_r = gctx[:, :, :].rearrange("p k (b t) -> p t k b", b=B)
        with tc.tile_pool(name="wrec", bufs=1) as wrec, \
             tc.tile_pool(name="smr", bufs=2) as smr, \
             tc.tile_pool(name="prc", bufs=1, space="PSUM") as prc, \
             tc.tile_pool(name="pr6", bufs=1, space="PSUM") as pr6, \
             tc.tile_pool(name="prs", bufs=1, space="PSUM") as prs, \
             tc.tile_pool(name="pou", bufs=1, space="PSUM") as pou:

            Whu = wrec.tile([128, KD, HID], BF, tag="Whu")
            nc.sync.dma_start(out=Whu, in_=r3(Whud))
            V1w = wrec.tile([128, MH, D], BF, tag="V1w")
            nc.sync.dma_start(out=V1w, in_=r3(V1d))
            Wtf = wrec.tile([128, KD, D], BF, tag="Wtf")
            nc.sync.dma_start(out=Wtf, in_=r3(Wtfd))
            cw1 = wrec.tile([128, KD, D], BF, tag="cw1")
            nc.sync.dma_start(out=cw1, in_=r3(cw1d))
            cw2 = wrec.tile([128, KD, D], BF, tag="cw2")
            nc.sync.dma_start(out=cw2, in_=r3(cw2d))
            gw2 = wrec.tile([128, KD, D], BF, tag="gw2")
            nc.sync.dma_start(out=gw2, in_=r3(gw2d))
            b1S = smr.tile([128, KD], F32, tag="b1S", bufs=1)
            nc.sync.dma_start(out=b1S, in_=b1d[:, :])
            cb1S = smr.tile([128, KD], F32, tag="cb1S", bufs=1)
            nc.sync.dma_start(out=cb1S, in_=cb1d[:, :])
            cb2S = smr.tile([128, KD], F32, tag="cb2S", bufs=1)
            nc.sync.dma_start(out=cb2S, in_=cb2d[:, :])
            # bf16 broadcast copies (B cols) for PSUM bias injection
            b1b = smr.tile([128, KD, B], BF, tag="b1b", bufs=1)
            nc.vector.tensor_copy(b1b, bc(b1S[:, :], B, 2))
            cb1b = smr.tile([128, KD, B], BF, tag="cb1b", bufs=1)
            nc.vector.tensor_copy(cb1b, bc(cb1S[:, :], B, 2))
            cb2b = smr.tile([128, KD, B], BF, tag="cb2b", bufs=1)
            nc.vector.tensor_copy(cb2b, bc(cb2S[:, :], B, 2))
            outgS = smr.tile([128, KD], F32, tag="outgS", bufs=1)
            nc.sync.dma_start(out=outgS, in_=outgd[:, :])
            outbS = smr.tile([128, KD], F32, tag="outbS", bufs=1)
            nc.sync.dma_start(out=outbS, in_=outbd[:, :])
            Hc = [wrec.tile([128, KD, 128], BF, tag=f"H{mt}", name=f"H{mt}")
                  for mt in range(MT)]

            import os as _os
            T_RUN = int(_os.environ.get("T_TRUNC", T))
            if T_RUN < T:
                for Hcm in Hc:
                    nc.vector.memset(Hcm, 0.0)

            NBLK = (T_RUN + 7) // 8
            ucb = []

            def uc_fetch(mt):
                tile_ = wrec.tile([128, MH, 128], BF, tag="ucb", bufs=2,
                                  name=f"ucb{mt}")
                nc.sync.dma_start(out=tile_, in_=ucT[:, :, mt * 128:(mt + 1) * 128])
                ucb.append(tile_)

            uc_fetch(0)
            if KDEBUG:
                nc.sync.dma_start(out=dbg_ctx[:, :],
                                  in_=ctxF[:, :, :].rearrange("p k n -> p (k n)"))
                nc.sync.dma_start(out=dbg_uc[:, :],
                                  in_=ucb[0][:, :, :].rearrange("p m c -> p (m c)"))

            # lm chunk emission: during step t of block w, process chunks of
            # block w-1 (Hc[w-1] is complete); 13 chunks spread over 8 steps.
            LM_SPLIT = [0, 2, 4, 6, 8, 10, 11, 12, 13]

            def lm_chunks(mt, j0, j1):
                for j in range(j0, j1):
                    lw = wrec.tile([128, KD, 512], BF, tag="lmw", bufs=3)
                    nc.sync.dma_start(
                        out=lw,
                        in_=lmTd[:, j * 512:(j + 1) * 512].rearrange("(k p) n -> p k n",
                                                                     p=128))
                    ps = pou.tile([128, 512], F32, tag="plm", bufs=2)
                    for k in range(KD):
                        nc.tensor.matmul(ps, Hc[mt][:, k, :], lw[:, k, :],
                                         start=(k == 0), stop=(k == KD - 1))
                    ot = smr.tile([128, 512], F32, tag="otile", bufs=3)
                    if j % 2 == 0:
                        nc.scalar.activation(ot, ps, AF.Copy)
                    else:
                        nc.vector.tensor_copy(ot, ps)
                    nc.sync.dma_start(out=outd[mt * 128:(mt + 1) * 128,
                                               j * 512:(j + 1) * 512], in_=ot)

            for t in range(T_RUN):
                ctx_t = ctx_r[:, t]
                mtb, off = t // 8, (t % 8) * B
                if t % 8 == 0 and mtb + 1 < NBLK:
                    uc_fetch(mtb + 1)
                if t > 0:
                    hprev = Hc[(t - 1) // 8][:, :, ((t - 1) % 8) * B:((t - 1) % 8) * B + B]
                u = prc.tile([128, MH, B], F32, tag="pu")
                for m in range(MH):
                    nc.tensor.matmul(u[:, m, :], eye, ucb[mtb][:, m, off:off + B],
                                     start=True, stop=(t == 0))
                    if t > 0:
                        for k in range(KD):
                            nc.tensor.matmul(u[:, m, :], Whu[:, k, m * 128:(m + 1) * 128],
                                             hprev[:, k, :], start=False,
                                             stop=(k == KD - 1))
                hsS = smr.tile([128, MH, B], BF, tag="hsS")
                nc.scalar.activation(hsS, u, AF.Gelu)
                # tf branch (off critical path): w = tf - ctx
                w_t = smr.tile([128, KD, B], F32, tag="w_t")
                if t > 0:
                    tfp = prc.tile([128, KD, B], F32, tag="ptf")
                    for m in range(KD):
                        for k in range(KD):
                            nc.tensor.matmul(tfp[:, m, :], Wtf[:, k, m * 128:(m + 1) * 128],
                                             hprev[:, k, :], start=(k == 0),
                                             stop=(k == KD - 1))
                    nc.vector.tensor_sub(w_t, tfp, ctx_t)
                else:
                    nc.vector.tensor_scalar_mul(w_t, ctx_t, -1.0)
                cpr = pr6.tile([128, KD, B], F32, tag="p6")
                for m in range(KD):
                    for k in range(MH):
                        nc.tensor.matmul(cpr[:, m, :], V1w[:, k, m * 128:(m + 1) * 128],
                                         hsS[:, k, :], start=(k == 0), stop=False)
                    nc.tensor.matmul(cpr[:, m, :], eye, b1b[:, m, :],
                                     start=False, stop=True)
                # cpf stored b-major so per-b layernorm views are contiguous
                # (gpsimd firmware requires contiguous [128, F] rows)
                cpf = smr.tile([128, B, KD], F32, tag="cpf")
                nc.scalar.activation(cpf[:, :, :].rearrange("p b k -> p k b"),
                                     cpr, AF.Gelu)
                # l2norm via gpsimd rmsnorm (no act-table); 1/sqrt(D) folded
                # into cw1 host-side.
                corePool = smr.tile([128, B, KD], F32, tag="corePool")
                for b in range(B):
                    nc.gpsimd.layernorm(corePool[:, b, :], cpf[:, b, :],
                                        subtract_mean=False, eps=1e-10)
                coreS = smr.tile([128, KD, B], BF, tag="coreS")
                nc.vector.tensor_copy(coreS,
                                      corePool[:, :, :].rearrange("p b k -> p k b"))
                gm = pr6.tile([128, KD, B], F32, tag="p6")
                for m in range(KD):
                    for k in range(KD):
                        nc.tensor.matmul(gm[:, m, :], cw1[:, k, m * 128:(m + 1) * 128],
                                         coreS[:, k, :], start=(k == 0), stop=False)
                    nc.tensor.matmul(gm[:, m, :], eye, cb1b[:, m, :],
                                     start=False, stop=True)
                gmS = smr.tile([128, KD, B], BF, tag="gmS")
                nc.scalar.activation(gmS, gm, AF.Gelu)
                cfp = pr6.tile([128, KD, B], F32, tag="p6")
                for m in range(KD):
                    for k in range(KD):
                        nc.tensor.matmul(cfp[:, m, :], cw2[:, k, m * 128:(m + 1) * 128],
                                         gmS[:, k, :], start=(k == 0), stop=False)
                    nc.tensor.matmul(cfp[:, m, :], eye, cb2b[:, m, :],
                                     start=False, stop=True)
                cfF = smr.tile([128, KD, B], F32, tag="cfF")
                nc.scalar.activation(cfF, cfp, AF.Copy)
                # gate cf-branch folded through cp_w2: reads g1 (gmS) directly,
                # in parallel with the cfp/cfF branch
                gp = pr6.tile([128, KD, B], F32, tag="pgp")
                for m in range(KD):
                    for k in range(KD):
                        nc.tensor.matmul(gp[:, m, :], gw2[:, k, m * 128:(m + 1) * 128],
                                         gmS[:, k, :], start=(k == 0), stop=False)
                    nc.tensor.matmul(gp[:, m, :], eye, gctx_r[:, t, m, :],
                                     start=False, stop=True)
                # gate via tanh (shares the gelu act table):
                #   sigmoid(x) = 0.5*(1 + tanh(x/2))
                #   hp = gate*(cf+tf-ctx) + ctx = 0.5*(tanh+1)*(cf+tf-ctx) + ctx
                tG = smr.tile([128, KD, B], F32, tag="gateS")
                nc.scalar.activation(tG, gp, AF.Tanh, scale=0.5)
                a1 = smr.tile([128, KD, B], F32, tag="a1")
                nc.vector.tensor_add(a1, cfF, w_t)
                q1 = smr.tile([128, KD, B], F32, tag="q1")
                nc.vector.scalar_tensor_tensor(q1, tG, 1.0, a1, OP.add, OP.mult)
                hp = smr.tile([128, B, KD], F32, tag="hp")
                nc.vector.scalar_tensor_tensor(
                    hp[:, :, :].rearrange("p b k -> p k b"),
                    q1, 0.5, ctx_t, OP.mult, OP.add)
                # LN + affine via gpsimd layernorm (no act-table), then clip.
                lnout = smr.tile([128, B, KD], F32, tag="lnout")
                for b in range(B):
                    nc.gpsimd.layernorm(lnout[:, b, :], hp[:, b, :],
                                        gamma_ap=outgS[:, :], beta_ap=outbS[:, :],
                                        subtract_mean=True, eps=1e-5)
                nc.vector.tensor_scalar(Hc[t // 8][:, :, (t % 8) * B:(t % 8) * B + B],
                                        lnout[:, :, :].rearrange("p b k -> p k b"),
                                        5.0, -5.0, OP.min, OP.max)
                if KDEBUG and t == min(7, T_RUN - 1):
                    nc.sync.dma_start(out=dbg_h[:, :],
                                      in_=Hc[0][:, :, :].rearrange("p k c -> p (k c)"))
                if KDEBUG and t == int(_os.environ.get("KPROBE_T", 0)):
                    for nm, tile_ in [("hs", hsS), ("cpf", cpf), ("core", coreS),
                                      ("gm", gmS), ("cf", cfF), ("tg", tG),
                                      ("hp", hp), ("ln", lnout)]:
                        nc.sync.dma_start(
                            out=dbg_st[nm][:, :],
                            in_=tile_[:, :, :].rearrange("p a b -> p (a b)"))
                # interleave lm-head work for the previous (complete) block
                if t >= 8 and T_RUN == T:
                    lm_chunks(mtb - 1, LM_SPLIT[t % 8], LM_SPLIT[t % 8 + 1])

            # lm-head tail: last block (or all blocks on truncated debug runs)
            tail_blocks = range(NBLK - 1, NBLK) if T_RUN == T else range(NBLK)
            for mt in tail_blocks:
                lm_chunks(mt, 0, NCH)

    nc.finalize()
    return nc


_NC_CACHE = {}


def _get_nc():
    key = (_FLAGS["ln_triv"], _FLAGS["bv0"])
    if key not in _NC_CACHE:
        _NC_CACHE[key] = build_nc()
    return _NC_CACHE[key]


def _prep_inputs(inputs):
    f = lambda x: np.asarray(x, np.float32)
    tok = np.asarray(inputs["token_ids"]).astype(np.int64)
    emb, pos = f(inputs["emb"]), f(inputs["pos_emb"])
    x0 = emb[tok.reshape(-1)] + np.tile(pos[:T], (B, 1))
    com = {"x0T": x0.T.astype(bfnp)}
    aiw, aib = f(inputs["attn_in_w"]), f(inputs["attn_in_b"])
    aow, aob = f(inputs["attn_out_w"]), f(inputs["attn_out_b"])
    for l in range(NL):
        wqk = np.zeros((D, 2048), np.float32)
        bqk = np.zeros(2048, np.float32)
        wv = np.zeros((D, 1024), np.float32)
        bv = np.zeros(1024, np.float32)
        wao = np.zeros((1024, D), np.float32)
        for h in range(NH):
            wqk[:, h * 128:h * 128 + HD] = aiw[l, h * HD:(h + 1) * HD, :].T
            wqk[:, 1024 + h * 128:1024 + h * 128 + HD] = aiw[l, D + h * HD:D + (h + 1) * HD, :].T
            bqk[h * 128:h * 128 + HD] = aib[l, h * HD:(h + 1) * HD]
            bqk[1024 + h * 128:1024 + h * 128 + HD] = aib[l, D + h * HD:D + (h + 1) * HD]
            wv[:, h * 128:h * 128 + HD] = aiw[l, 2 * D + h * HD:2 * D + (h + 1) * HD, :].T
            bv[h * 128:h * 128 + HD] = aib[l, 2 * D + h * HD:2 * D + (h + 1) * HD]
            wao[h * 128:h * 128 + HD, :] = aow[l][:, h * HD:(h + 1) * HD].T
        com[f"wqk{l}"] = wqk.astype(bfnp)
        com[f"bqk{l}"] = bqk.reshape(16, 128).T.copy()
        com[f"wv{l}"] = wv.astype(bfnp)
        com[f"bv{l}"] = bv.reshape(1, 1024).astype(bfnp)
        com[f"wao{l}"] = wao.astype(bfnp)
        com[f"bao{l}"] = aob[l].reshape(KD, 128).T.copy()
        com[f"ff1T{l}"] = f(inputs["ff_w1"])[l].T.astype(bfnp).copy()
        com[f"bff1{l}"] = f(inputs["ff_b1"])[l].reshape(MF, 128).T.copy()
        com[f"ff2T{l}"] = f(inputs["ff_w2"])[l].T.astype(bfnp).copy()
        com[f"bff2{l}"] = f(inputs["ff_b2"])[l].reshape(KD, 128).T.copy()
        com[f"g1{l}"] = f(inputs["n1_g"])[l].reshape(KD, 128).T.copy()
        com[f"bb1{l}"] = f(inputs["n1_b"])[l].reshape(KD, 128).T.copy()
        com[f"g2{l}"] = f(inputs["n2_g"])[l].reshape(KD, 128).T.copy()
        com[f"bb2{l}"] = f(inputs["n2_b"])[l].reshape(KD, 128).T.copy()
    com["encg"] = f(inputs["enc_norm_g"]).reshape(KD, 128).T.copy()
    com["encb"] = f(inputs["enc_norm_b"]).reshape(KD, 128).T.copy()
    tk, tq = np.meshgrid(np.arange(64), np.arange(64), indexing="ij")
    com["maskT"] = ((tk > tq) * -30000.0).astype(bfnp)
    com["eyed"] = np.eye(128, dtype=bfnp)
    com["V0d"] = f(inputs["V0"]).astype(bfnp)
    com["V1d"] = f(inputs["V1"]).astype(bfnp)
    R, tw = f(inputs["R"]), f(inputs["temp_w"])
    # folded recurrent weights: u += (alpha*R@V0)^T h ; tf = (alpha*R@tw^T)^T h
    com["Whud"] = (ALPHA * R @ f(inputs["V0"])).astype(bfnp)
    com["Wtfd"] = (ALPHA * R @ tw.T).astype(bfnp)
    # rmsnorm(x) = sqrt(D) * l2norm(x): fold the 1/sqrt(D) into cp_w1
    com["cw1d"] = (f(inputs["cp_w1"]).T / np.sqrt(D)).astype(bfnp).copy()
    com["cw2d"] = f(inputs["cp_w2"]).T.astype(bfnp).copy()
    gw = f(inputs["gate_w"])
    com["gw1d"] = gw[:, :D].T.astype(bfnp).copy()
    # gate cf-branch folded through cp_w2: gs_cf = Wgg^T g1 (+ gwB@cp_b2 -> gb)
    gwB = gw[:, D:]
    com["gw2d"] = np.ascontiguousarray((gwB @ f(inputs["cp_w2"])).T).astype(bfnp)
    com["gbd"] = (f(inputs["gate_b"]) + gwB @ f(inputs["cp_b2"])).reshape(KD, 128).T.copy()
    com["b0d"] = f(inputs["b0"]).reshape(MH, 128).T.copy()
    com["b1d"] = f(inputs["b1"]).reshape(KD, 128).T.copy()
    com["cb1d"] = f(inputs["cp_b1"]).reshape(KD, 128).T.copy()
    com["cb2d"] = f(inputs["cp_b2"]).reshape(KD, 128).T.copy()
    com["outgd"] = np.ascontiguousarray(f(inputs["out_g"]).reshape(KD, 128).T)
    com["outbd"] = np.ascontiguousarray(f(inputs["out_b"]).reshape(KD, 128).T)
    lmp = np.zeros((VP, D), np.float32)
    lmp[:V] = f(inputs["lm_head"])
    lmT = lmp.T.astype(bfnp)
    shards = [np.ascontiguousarray(lmT[:, c * VS:(c + 1) * VS]) for c in range(8)]
    return com, shards


LAST_RESULT = {}


def kernel(**inputs):
    import os
    f = lambda x: np.asarray(x, np.float32)
    _FLAGS["ln_triv"] = bool(
        all(np.all(f(inputs[k]) == 1.0) for k in ("n1_g", "n2_g", "enc_norm_g"))
        and all(np.all(f(inputs[k]) == 0.0) for k in ("n1_b", "n2_b", "enc_norm_b")))
    _FLAGS["bv0"] = bool(np.all(f(inputs["attn_in_b"])[:, 2 * D:] == 0.0))
    nc = _get_nc()
    com, shards = _prep_inputs(inputs)
    in_maps = [{**com, "lmTd": shards[c]} for c in range(8)]
    kw = {}
    if os.environ.get("KTRACE"):
        kw = dict(trace=True, tmpdir=os.environ.get("KTRACE_DIR", "/root/problem/trace_out"))
    res = run_bass_kernel_spmd(nc, in_maps, core_ids=list(range(8)), **kw)
    LAST_RESULT["res"] = res
    parts = [res.results[c]["out"] for c in range(8)]          # each [1024, VS], t-major rows
    full = np.concatenate(parts, axis=1)[:, :V]                # [1024, V]
    return np.ascontiguousarray(full.reshape(T, B, V).transpose(1, 0, 2))



# revision 54
# speedup vs baseline: 1.4158x; 1.0312x over previous
"""Trainium2 Bass kernel for nn_AgnisV5: 2-layer GPT encoder + gated
hierarchical recurrence + lm_head, SPMD over 8 NeuronCores.

Strategy: encoder + recurrence replicated on all cores (no collectives);
lm_head vocab-sharded 8 ways. Forward-pass simplification: stop_gradient is
identity, so the "settled" hierarchy path equals the differentiable shadow
path and blend == core_sh.

Layouts: feature-major activations [128p, K, N] (feature f = k*128 + p).
Encoder token columns are b-major (n = b*64 + t); recurrence/H/lm_head use
t-major (n = t*16 + b) via strided views of ctx.
"""
import numpy as np
import ml_dtypes
from contextlib import ExitStack

import concourse.bass as bass
import concourse.mybir as mybir
import concourse.tile as tile
from concourse import bacc
from concourse.bass_utils import run_bass_kernel_spmd

AF = mybir.ActivationFunctionType
OP = mybir.AluOpType
BF = mybir.dt.float16
F32 = mybir.dt.float32

D, HID, FFN, NH, NL, T, V, B = 768, 3072, 2048, 8, 2, 64, 50257, 16
HD = 96
KD = D // 128          # 6
MH = HID // 128        # 24
MF = FFN // 128        # 16
NTOK = B * T           # 1024
MT = NTOK // 128       # 8
VP = 53248             # padded vocab = 8*6656
VS = VP // 8           # 6656 per core
NCH = VS // 512        # 13
ALPHA = 0.4
QK_SCALE = 1.0 / np.sqrt(96.0)

bfnp = np.float16


_FLAGS = {"ln_triv": False, "bv0": False}


def bc(ap, reps, pos):
    """Insert a stride-0 (broadcast) free dim at position pos (1-based over free dims)."""
    newap = list(ap.ap)
    newap.insert(pos, [0, reps])
    return bass.AP(tensor=ap.tensor, offset=ap.offset, ap=newap)


def build_nc():
    LN_TRIV = _FLAGS["ln_triv"]
    BV0 = _FLAGS["bv0"]
    nc = bacc.Bacc(None, target_bir_lowering=False)

    # ---- DRAM I/O ----
    x0T = nc.dram_tensor("x0T", [D, NTOK], BF, kind="ExternalInput")
    enc_in = {}
    for l in range(NL):
        enc_in[f"wqk{l}"] = nc.dram_tensor(f"wqk{l}", [D, 2048], BF, kind="ExternalInput")
        enc_in[f"wv{l}"] = nc.dram_tensor(f"wv{l}", [D, 1024], BF, kind="ExternalInput")
        enc_in[f"wao{l}"] = nc.dram_tensor(f"wao{l}", [1024, D], BF, kind="ExternalInput")
        enc_in[f"ff1T{l}"] = nc.dram_tensor(f"ff1T{l}", [D, FFN], BF, kind="ExternalInput")
        enc_in[f"ff2T{l}"] = nc.dram_tensor(f"ff2T{l}", [FFN, D], BF, kind="ExternalInput")
        enc_in[f"bqk{l}"] = nc.dram_tensor(f"bqk{l}", [128, 16], F32, kind="ExternalInput")
        enc_in[f"bv{l}"] = nc.dram_tensor(f"bv{l}", [1, 1024], BF, kind="ExternalInput")
        enc_in[f"bao{l}"] = nc.dram_tensor(f"bao{l}", [128, KD], F32, kind="ExternalInput")
        enc_in[f"bff1{l}"] = nc.dram_tensor(f"bff1{l}", [128, MF], F32, kind="ExternalInput")
        enc_in[f"bff2{l}"] = nc.dram_tensor(f"bff2{l}", [128, KD], F32, kind="ExternalInput")
        enc_in[f"g1{l}"] = nc.dram_tensor(f"g1{l}", [128, KD], F32, kind="ExternalInput")
        enc_in[f"bb1{l}"] = nc.dram_tensor(f"bb1{l}", [128, KD], F32, kind="ExternalInput")
        enc_in[f"g2{l}"] = nc.dram_tensor(f"g2{l}", [128, KD], F32, kind="ExternalInput")
        enc_in[f"bb2{l}"] = nc.dram_tensor(f"bb2{l}", [128, KD], F32, kind="ExternalInput")
    encg = nc.dram_tensor("encg", [128, KD], F32, kind="ExternalInput")
    encb = nc.dram_tensor("encb", [128, KD], F32, kind="ExternalInput")
    maskT = nc.dram_tensor("maskT", [64, 64], BF, kind="ExternalInput")
    V0d = nc.dram_tensor("V0d", [D, HID], BF, kind="ExternalInput")
    V1d = nc.dram_tensor("V1d", [HID, D], BF, kind="ExternalInput")
    Whud = nc.dram_tensor("Whud", [D, HID], BF, kind="ExternalInput")
    Wtfd = nc.dram_tensor("Wtfd", [D, D], BF, kind="ExternalInput")
    cw1d = nc.dram_tensor("cw1d", [D, D], BF, kind="ExternalInput")
    cw2d = nc.dram_tensor("cw2d", [D, D], BF, kind="ExternalInput")
    gw1d = nc.dram_tensor("gw1d", [D, D], BF, kind="ExternalInput")
    gw2d = nc.dram_tensor("gw2d", [D, D], BF, kind="ExternalInput")
    b0d = nc.dram_tensor("b0d", [128, MH], F32, kind="ExternalInput")
    b1d = nc.dram_tensor("b1d", [128, KD], F32, kind="ExternalInput")
    cb1d = nc.dram_tensor("cb1d", [128, KD], F32, kind="ExternalInput")
    cb2d = nc.dram_tensor("cb2d", [128, KD], F32, kind="ExternalInput")
    gbd = nc.dram_tensor("gbd", [128, KD], F32, kind="ExternalInput")
    outgd = nc.dram_tensor("outgd", [128, KD], F32, kind="ExternalInput")
    outbd = nc.dram_tensor("outbd", [128, KD], F32, kind="ExternalInput")
    lmTd = nc.dram_tensor("lmTd", [D, VS], BF, kind="ExternalInput")
    eyed = nc.dram_tensor("eyed", [128, 128], BF, kind="ExternalInput")
    outd = nc.dram_tensor("out", [NTOK, VS], BF, kind="ExternalOutput")
    import os as _os0
    KDEBUG = bool(_os0.environ.get("KDEBUG"))
    if KDEBUG:
        dbg_ctx = nc.dram_tensor("dbg_ctx", [128, KD * NTOK], BF, kind="ExternalOutput")
        dbg_uc = nc.dram_tensor("dbg_uc", [128, MH * 128], BF, kind="ExternalOutput")
        dbg_h = nc.dram_tensor("dbg_h", [128, KD * 128], BF, kind="ExternalOutput")
        dbg_st = {}
        for nm, width, dt_ in [("hs", MH * B, BF), ("cpf", KD * B, F32),
                               ("core", KD * B, BF), ("gm", KD * B, BF),
                               ("cf", KD * B, F32), ("tg", KD * B, F32),
                               ("hp", KD * B, F32), ("ln", KD * B, F32)]:
            dbg_st[nm] = nc.dram_tensor(f"dbg_{nm}", [128, width], dt_,
                                        kind="ExternalOutput")

    def r3(h, p=128):
        return h[:, :].rearrange("(k p) n -> p k n", p=p)

    with ExitStack() as ctx:
        tc = ctx.enter_context(tile.TileContext(nc))
        cross = ctx.enter_context(tc.tile_pool(name="cross", bufs=1))

        # constants (cross-scope)
        ones_col = cross.tile([128, 1], BF, tag="ones_col")
        nc.vector.memset(ones_col, 1.0)
        ones64 = cross.tile([64, 1], BF, tag="ones64")
        nc.vector.memset(ones64, 1.0)
        ones_row_f = cross.tile([1, 128], F32, tag="ones_row_f")
        nc.vector.memset(ones_row_f, 1.0)
        ones_r64f = cross.tile([1, 64], F32, tag="ones_r64f")
        nc.vector.memset(ones_r64f, 1.0)
        mask_sb = cross.tile([64, 64], BF, tag="mask")
        nc.sync.dma_start(out=mask_sb, in_=maskT[:, :])
        eye = cross.tile([128, 128], BF, tag="eye")
        nc.sync.dma_start(out=eye, in_=eyed[:, :])
        eps5 = cross.tile([128, 1], F32, tag="eps5")
        nc.vector.memset(eps5, 1e-5)
        eps24 = cross.tile([128, 1], F32, tag="eps24")
        nc.vector.memset(eps24, 1e-24)
        ctxF = cross.tile([128, KD, NTOK], BF, tag="ctxF")

        # ================= ENCODER SCOPE =================
        with tc.tile_pool(name="wenc", bufs=1) as wenc, \
             tc.tile_pool(name="bige", bufs=1) as bige, \
             tc.tile_pool(name="sme", bufs=1) as sme, \
             tc.tile_pool(name="pmm", bufs=2, space="PSUM") as pmm, \
             tc.tile_pool(name="pst", bufs=1, space="PSUM") as pst, \
             tc.tile_pool(name="pbc", bufs=2, space="PSUM") as pbc:

            def ln_fm(x_sb, g_ap, b_ap, out, l2_out=None, triv=False):
                """x_sb [128, KD, 1024] bf16 -> LN -> out; optional l2 -> l2_out.
                triv: gamma==1 and beta==0 (runtime-specialized)."""
                for j in range(2):
                    jj = slice(j * 512, (j + 1) * 512)
                    s12 = pst.tile([1, 2, 512], F32, tag="s1")
                    for k in range(KD):
                        sq = bige.tile([128, 512], BF, tag="sqc", bufs=2)
                        nc.vector.tensor_mul(sq, x_sb[:, k, jj], x_sb[:, k, jj])
                        nc.tensor.matmul(s12[:, 0, :], ones_col, x_sb[:, k, jj],
                                         start=(k == 0), stop=(k == KD - 1))
                        nc.tensor.matmul(s12[:, 1, :], ones_col, sq,
                                         start=(k == 0), stop=(k == KD - 1))
                    m = sme.tile([1, 512], F32, tag="mtag", bufs=2)
                    nc.vector.tensor_scalar_mul(m, s12[:, 0, :], 1.0 / D)
                    msq = sme.tile([1, 512], F32, tag="scr512", bufs=2)
                    nc.vector.tensor_mul(msq, m, m)
                    var = sme.tile([1, 512], F32, tag="scr512", bufs=2)
                    nc.vector.scalar_tensor_tensor(var, s12[:, 1, :], 1.0 / D,
                                                   msq, OP.mult, OP.subtract)
                    sd = sme.tile([1, 512], F32, tag="scr512", bufs=2)
                    nc.scalar.activation(sd, var, AF.Sqrt, bias=eps5[:1, :])
                    ac = sme.tile([1, 2, 512], F32, tag="acf")
                    nc.vector.reciprocal(ac[:, 0, :], sd)
                    nc.vector.scalar_tensor_tensor(ac[:, 1, :], m, -1.0,
                                                   ac[:, 0, :], OP.mult, OP.mult)
                    pac = pbc.tile([128, 2, 512], F32, tag="pabc", bufs=1)
                    for a in range(2):
                        nc.tensor.matmul(pac[:, a, :], ones_row_f, ac[:, a, :],
                                         start=True, stop=True)
                    pab = bige.tile([128, 2, 512], BF, tag="pab16", bufs=2)
                    nc.scalar.activation(pab, pac, AF.Copy)
                    for k in range(KD):
                        if triv:
                            t1 = bige.tile([128, 512], BF, tag="t1", bufs=2)
                            nc.vector.tensor_mul(t1, x_sb[:, k, jj], pab[:, 0, :])
                            nc.vector.tensor_add(out[:, k, jj], t1, pab[:, 1, :])
                        else:
                            t1 = bige.tile([128, 512], F32, tag="t1f", bufs=2)
                            nc.vector.tensor_mul(t1, x_sb[:, k, jj], pab[:, 0, :])
                            nc.vector.tensor_add(t1, t1, pab[:, 1, :])
                            nc.vector.tensor_scalar(out[:, k, jj], t1,
                                                    g_ap[:, k:k + 1],
                                                    b_ap[:, k:k + 1],
                                                    OP.mult, OP.add)
                if l2_out is None:
                    return
                for j in range(2):
                    jj = slice(j * 512, (j + 1) * 512)
                    s1t = pst.tile([1, 2, 512], F32, tag="s1")
                    s1 = s1t[:, 0, :]
                    for k in range(KD):
                        sq = bige.tile([128, 512], BF, tag="sqc", bufs=2)
                        nc.vector.tensor_mul(sq, out[:, k, jj], out[:, k, jj])
                        nc.tensor.matmul(s1, ones_col, sq,
                                         start=(k == 0), stop=(k == KD - 1))
                    sd = sme.tile([1, 512], F32, tag="scr512", bufs=2)
                    nc.scalar.activation(sd, s1, AF.Sqrt, bias=eps24[:1, :])
                    rr = sme.tile([1, 512], F32, tag="scr512", bufs=2)
                    nc.vector.reciprocal(rr, sd)
                    pat = pbc.tile([128, 2, 512], F32, tag="pabc", bufs=1)
                    pa = pat[:, 0, :]
                    nc.tensor.matmul(pa, ones_row_f, rr, start=True, stop=True)
                    pa16 = bige.tile([128, 512], BF, tag="pa16", bufs=2)
                    nc.scalar.activation(pa16, pa, AF.Copy)
                    for k in range(KD):
                        nc.vector.tensor_mul(l2_out[:, k, jj], out[:, k, jj], pa16)

            X = bige.tile([128, KD, NTOK], BF, tag="X")
            nc.sync.dma_start(out=X, in_=r3(x0T))
            gA = sme.tile([128, NL, KD], F32, tag="gA")
            bA = sme.tile([128, NL, KD], F32, tag="bA")
            gB = sme.tile([128, NL, KD], F32, tag="gB")
            bB = sme.tile([128, NL, KD], F32, tag="bB")
            for l in range(NL):
                nc.sync.dma_start(out=gA[:, l, :], in_=enc_in[f"g1{l}"][:, :])
                nc.sync.dma_start(out=bA[:, l, :], in_=enc_in[f"bb1{l}"][:, :])
                nc.sync.dma_start(out=gB[:, l, :], in_=enc_in[f"g2{l}"][:, :])
                nc.sync.dma_start(out=bB[:, l, :], in_=enc_in[f"bb2{l}"][:, :])

            for l in range(NL):
                XN = bige.tile([128, KD, NTOK], BF, tag="XN")
                ln_fm(X, gA[:, l, :], bA[:, l, :], XN, triv=LN_TRIV)
                wqk = wenc.tile([128, KD, 2048], BF, tag="wbig")
                nc.sync.dma_start(out=wqk, in_=r3(enc_in[f"wqk{l}"]))
                bqk = sme.tile([128, 16], F32, tag="bqk")
                nc.sync.dma_start(out=bqk, in_=enc_in[f"bqk{l}"][:, :])
                # v projection (token-major)
                wv = wenc.tile([128, KD, 1024], BF, tag="wsmall")
                nc.sync.dma_start(out=wv, in_=r3(enc_in[f"wv{l}"]))
                if not BV0:
                    ones_row = sme.tile([1, 128], BF, tag="ones_row")
                    nc.vector.memset(ones_row, 1.0)
                    bvr = sme.tile([1, 1024], BF, tag="bvr")
                    nc.sync.dma_start(out=bvr, in_=enc_in[f"bv{l}"][:, :])
                    bvs = bige.tile([128, 1024], BF, tag="bvs")
                    for j in range(2):
                        pb = pbc.tile([128, 2, 512], F32, tag="pabc", bufs=1)
                        nc.tensor.matmul(pb[:, 0, :], ones_row,
                                         bvr[:, j * 512:(j + 1) * 512],
                                         start=True, stop=True)
                        nc.vector.tensor_copy(bvs[:, j * 512:(j + 1) * 512],
                                              pb[:, 0, :])
                vS = bige.tile([64, B, 1024], BF, tag="vS")
                for b in range(B):
                    ps = pmm.tile([64, 1024], F32, tag="pmm")
                    for k in range(KD):
                        for j in range(2):
                            nc.tensor.matmul(ps[:, j * 512:(j + 1) * 512],
                                             XN[:, k, b * 64:(b + 1) * 64],
                                             wv[:, k, j * 512:(j + 1) * 512],
                                             start=(k == 0), stop=(k == KD - 1))
                    if not BV0:
                        nc.vector.tensor_add(vS[:, b, :], ps, bvs[:64, :])
                    elif b % 2 == 0:
                        nc.scalar.activation(vS[:, b, :], ps, AF.Copy)
                    else:
                        nc.vector.tensor_copy(vS[:, b, :], ps)
                # attention per head; additive mask injected into score PSUM
                ofS = bige.tile([128, NH, NTOK], BF, tag="ofS")
                for h in range(NH):
                    qh = bige.tile([128, NTOK], BF, tag="qh", bufs=1)
                    ph = pmm.tile([128, 1024], F32, tag="pmm")
                    for k in range(KD):
                        for j in range(2):
                            nc.tensor.matmul(ph[:, j * 512:(j + 1) * 512],
                                             wqk[:, k, h * 128:(h + 1) * 128],
                                             XN[:, k, j * 512:(j + 1) * 512],
                                             start=(k == 0), stop=(k == KD - 1))
                    nc.vector.tensor_scalar_add(qh, ph, bqk[:, h:h + 1])
                    kh = bige.tile([128, NTOK], BF, tag="kh", bufs=1)
                    ph2 = pmm.tile([128, 1024], F32, tag="pmm")
                    for k in range(KD):
                        for j in range(2):
                            nc.tensor.matmul(ph2[:, j * 512:(j + 1) * 512],
                                             wqk[:, k, 1024 + h * 128:1024 + (h + 1) * 128],
                                             XN[:, k, j * 512:(j + 1) * 512],
                                             start=(k == 0), stop=(k == KD - 1))
                    nc.scalar.activation(kh, ph2, AF.Identity,
                                         bias=bqk[:, 8 + h:9 + h])
                    sT = pmm.tile([64, 1024], F32, tag="pmm")
                    for b in range(B):
                        nc.tensor.matmul(sT[:, b * 64:(b + 1) * 64],
                                         kh[:, b * 64:(b + 1) * 64],
                                         qh[:, b * 64:(b + 1) * 64],
                                         start=True, stop=False)
                        nc.tensor.matmul(sT[:, b * 64:(b + 1) * 64],
                                         eye[:64, :64], mask_sb,
                                         start=False, stop=True)
                    eT = bige.tile([64, NTOK], BF, tag="eT", bufs=2)
                    nc.scalar.activation(eT, sT, AF.Exp, scale=QK_SCALE)
                    ssum = pst.tile([1, 2, 512], F32, tag="s1")
                    for a in range(2):
                        nc.tensor.matmul(ssum[:, a, :], ones64,
                                         eT[:, a * 512:(a + 1) * 512],
                                         start=True, stop=True)
                    rs2 = sme.tile([1, 2, 512], F32, tag="acf")
                    rs = rs2[:, :, :].rearrange("p a c -> p (a c)")
                    nc.vector.reciprocal(rs, ssum[:, :, :].rearrange("p a c -> p (a c)"))
                    rb64 = pmm.tile([128, 1024], F32, tag="pmm")
                    for a in range(2):
                        nc.tensor.matmul(rb64[:64, a * 512:(a + 1) * 512],
                                         ones_r64f, rs[:, a * 512:(a + 1) * 512],
                                         start=True, stop=True)
                    eN = bige.tile([64, NTOK], BF, tag="eN", bufs=2)
                    nc.vector.tensor_mul(eN, eT, rb64[:64, :])
                    oh = pmm.tile([128, 1024], F32, tag="pmm")
                    for b in range(B):
                        nc.tensor.matmul(oh[:, b * 64:(b + 1) * 64],
                                         vS[:, b, h * 128:(h + 1) * 128],
                                         eN[:, b * 64:(b + 1) * 64],
                                         start=True, stop=True)
                    if h % 2 == 0:
                        nc.scalar.activation(ofS[:, h, :], oh, AF.Copy)
                    else:
                        nc.vector.tensor_copy(ofS[:, h, :], oh)
                # attn out + residual (in place on X)
                wao = wenc.tile([128, NH, D], BF, tag="wsmall")
                nc.sync.dma_start(out=wao,
                                  in_=enc_in[f"wao{l}"][:, :].rearrange("(k p) n -> p k n", p=128))
                bao = sme.tile([128, KD], F32, tag="bao")
                nc.sync.dma_start(out=bao, in_=enc_in[f"bao{l}"][:, :])
                for mo in range(KD):
                    ps = pmm.tile([128, 1024], F32, tag="pmm")
                    for kv in range(NH):
                        for j in range(2):
                            nc.tensor.matmul(ps[:, j * 512:(j + 1) * 512],
                                             wao[:, kv, mo * 128:(mo + 1) * 128],
                                             ofS[:, kv, j * 512:(j + 1) * 512],
                                             start=(kv == 0), stop=(kv == NH - 1))
                    t2 = bige.tile([128, 1024], F32, tag="t2", bufs=2)
                    nc.vector.tensor_scalar_add(t2, ps, bao[:, mo:mo + 1])
                    nc.vector.tensor_add(X[:, mo, :], X[:, mo, :], t2)
                # ffn
                XN2 = bige.tile([128, KD, NTOK], BF, tag="XN")
                ln_fm(X, gB[:, l, :], bB[:, l, :], XN2, triv=LN_TRIV)
                w1 = wenc.tile([128, KD, FFN], BF, tag="wbig")
                nc.sync.dma_start(out=w1, in_=r3(enc_in[f"ff1T{l}"]))
                bf1 = sme.tile([128, MF], F32, tag="bf1")
                nc.sync.dma_start(out=bf1, in_=enc_in[f"bff1{l}"][:, :])
                G = bige.tile([128, MF, NTOK], BF, tag="G")
                for mf in range(MF):
                    ps = pmm.tile([128, 1024], F32, tag="pmm")
                    for k in range(KD):
                        for j in range(2):
                            nc.tensor.matmul(ps[:, j * 512:(j + 1) * 512],
                                             w1[:, k, mf * 128:(mf + 1) * 128],
                                             XN2[:, k, j * 512:(j + 1) * 512],
                                             start=(k == 0), stop=(k == KD - 1))
                    nc.scalar.activation(G[:, mf, :], ps, AF.Gelu, bias=bf1[:, mf:mf + 1])
                w2 = wenc.tile([128, MF, D], BF, tag="wbig")
                nc.sync.dma_start(out=w2, in_=r3(enc_in[f"ff2T{l}"]))
                bf2 = sme.tile([128, KD], F32, tag="bao")
                nc.sync.dma_start(out=bf2, in_=enc_in[f"bff2{l}"][:, :])
                for mo in range(KD):
                    ps = pmm.tile([128, 1024], F32, tag="pmm")
                    for kf in range(MF):
                        for j in range(2):
                            nc.tensor.matmul(ps[:, j * 512:(j + 1) * 512],
                                             w2[:, kf, mo * 128:(mo + 1) * 128],
                                             G[:, kf, j * 512:(j + 1) * 512],
                                             start=(kf == 0), stop=(kf == MF - 1))
                    t2 = bige.tile([128, 1024], F32, tag="t2", bufs=2)
                    nc.vector.tensor_scalar_add(t2, ps, bf2[:, mo:mo + 1])
                    nc.vector.tensor_add(X[:, mo, :], X[:, mo, :], t2)

            # final norm + l2 -> ctxF (cross pool)
            eg = sme.tile([128, KD], F32, tag="eg")
            eb = sme.tile([128, KD], F32, tag="eb")
            nc.sync.dma_start(out=eg, in_=encg[:, :])
            nc.sync.dma_start(out=eb, in_=encb[:, :])
            pre = bige.tile([128, KD, NTOK], BF, tag="XN")
            ln_fm(X, eg, eb, pre, l2_out=ctxF, triv=LN_TRIV)


        # ================= RECURRENCE SCOPE =================
        ctx_r = ctxF[:, :, :].rearrange("p k (b t) -> p t k b", b=B)
        with tc.tile_pool(name="wrec", bufs=1) as wrec, \
             tc.tile_pool(name="smr", bufs=2) as smr, \
             tc.tile_pool(name="prc", bufs=1, space="PSUM") as prc, \
             tc.tile_pool(name="pr6", bufs=1, space="PSUM") as pr6, \
             tc.tile_pool(name="prs", bufs=1, space="PSUM") as prs, \
             tc.tile_pool(name="pou", bufs=1, space="PSUM") as pou:

            Whu = wrec.tile([128, KD, HID], BF, tag="Whu")
            nc.sync.dma_start(out=Whu, in_=r3(Whud))
            V0w = wrec.tile([128, KD, HID], BF, tag="V0w")
            nc.sync.dma_start(out=V0w, in_=r3(V0d))
            gw1w = wrec.tile([128, KD, D], BF, tag="gw1w")
            nc.sync.dma_start(out=gw1w, in_=r3(gw1d))
            V1w = wrec.tile([128, MH, D], BF, tag="V1w")
            nc.sync.dma_start(out=V1w, in_=r3(V1d))
            Wtf = wrec.tile([128, KD, D], BF, tag="Wtf")
            nc.sync.dma_start(out=Wtf, in_=r3(Wtfd))
            cw1 = wrec.tile([128, KD, D], BF, tag="cw1")
            nc.sync.dma_start(out=cw1, in_=r3(cw1d))
            cw2 = wrec.tile([128, KD, D], BF, tag="cw2")
            nc.sync.dma_start(out=cw2, in_=r3(cw2d))
            gw2 = wrec.tile([128, KD, D], BF, tag="gw2")
            nc.sync.dma_start(out=gw2, in_=r3(gw2d))
            b1S = smr.tile([128, KD], F32, tag="b1S", bufs=1)
            nc.sync.dma_start(out=b1S, in_=b1d[:, :])
            cb1S = smr.tile([128, KD], F32, tag="cb1S", bufs=1)
            nc.sync.dma_start(out=cb1S, in_=cb1d[:, :])
            cb2S = smr.tile([128, KD], F32, tag="cb2S", bufs=1)
            nc.sync.dma_start(out=cb2S, in_=cb2d[:, :])
            # bf16 broadcast copies (B cols) for PSUM bias injection
            b0S = smr.tile([128, MH], F32, tag="b0S", bufs=1)
            nc.sync.dma_start(out=b0S, in_=b0d[:, :])
            b0b = smr.tile([128, MH, B], BF, tag="b0b", bufs=1)
            nc.vector.tensor_copy(b0b, bc(b0S[:, :], B, 2))
            gbS = smr.tile([128, KD], F32, tag="gbS", bufs=1)
            nc.sync.dma_start(out=gbS, in_=gbd[:, :])
            gbb = smr.tile([128, KD, B], BF, tag="gbb", bufs=1)
            nc.vector.tensor_copy(gbb, bc(gbS[:, :], B, 2))
            b1b = smr.tile([128, KD, B], BF, tag="b1b", bufs=1)
            nc.vector.tensor_copy(b1b, bc(b1S[:, :], B, 2))
            cb1b = smr.tile([128, KD, B], BF, tag="cb1b", bufs=1)
            nc.vector.tensor_copy(cb1b, bc(cb1S[:, :], B, 2))
            cb2b = smr.tile([128, KD, B], BF, tag="cb2b", bufs=1)
            nc.vector.tensor_copy(cb2b, bc(cb2S[:, :], B, 2))
            outgS = smr.tile([128, KD], F32, tag="outgS", bufs=1)
            nc.sync.dma_start(out=outgS, in_=outgd[:, :])
            outbS = smr.tile([128, KD], F32, tag="outbS", bufs=1)
            nc.sync.dma_start(out=outbS, in_=outbd[:, :])
            Hc = [wrec.tile([128, KD, 128], BF, tag=f"H{mt}", name=f"H{mt}")
                  for mt in range(MT)]

            import os as _os
            T_RUN = int(_os.environ.get("T_TRUNC", T))
            if T_RUN < T:
                for Hcm in Hc:
                    nc.vector.memset(Hcm, 0.0)

            NBLK = (T_RUN + 7) // 8
            if KDEBUG:
                nc.sync.dma_start(out=dbg_ctx[:, :],
                                  in_=ctxF[:, :, :].rearrange("p k n -> p (k n)"))

            # lm chunk emission: during step t of block w, process chunks of
            # block w-1 (Hc[w-1] is complete); 13 chunks spread over 8 steps.
            LM_SPLIT = [0, 2, 4, 6, 8, 10, 11, 12, 13]

            def lm_chunks(mt, j0, j1):
                for j in range(j0, j1):
                    lw = wrec.tile([128, KD, 512], BF, tag="lmw", bufs=2)
                    nc.sync.dma_start(
                        out=lw,
                        in_=lmTd[:, j * 512:(j + 1) * 512].rearrange("(k p) n -> p k n",
                                                                     p=128))
                    ps = pou.tile([128, 512], F32, tag="plm", bufs=2)
                    for k in range(KD):
                        nc.tensor.matmul(ps, Hc[mt][:, k, :], lw[:, k, :],
                                         start=(k == 0), stop=(k == KD - 1))
                    ot = smr.tile([128, 512], BF, tag="otile", bufs=3)
                    if j % 2 == 0:
                        nc.scalar.activation(ot, ps, AF.Copy)
                    else:
                        nc.vector.tensor_copy(ot, ps)
                    nc.sync.dma_start(out=outd[mt * 128:(mt + 1) * 128,
                                               j * 512:(j + 1) * 512], in_=ot)

            for t in range(T_RUN):
                ctx_t = ctx_r[:, t]
                mtb, off = t // 8, (t % 8) * B
                if t > 0:
                    hprev = Hc[(t - 1) // 8][:, :, ((t - 1) % 8) * B:((t - 1) % 8) * B + B]
                u = prc.tile([128, MH, B], F32, tag="pu")
                for m in range(MH):
                    nc.tensor.matmul(u[:, m, :], eye, b0b[:, m, :],
                                     start=True, stop=False)
                    for k in range(KD):
                        nc.tensor.matmul(u[:, m, :], V0w[:, k, m * 128:(m + 1) * 128],
                                         ctx_t[:, k, :], start=False,
                                         stop=(t == 0 and k == KD - 1))
                    if t > 0:
                        for k in range(KD):
                            nc.tensor.matmul(u[:, m, :], Whu[:, k, m * 128:(m + 1) * 128],
                                             hprev[:, k, :], start=False,
                                             stop=(k == KD - 1))
                hsS = smr.tile([128, MH, B], BF, tag="hsS")
                nc.scalar.activation(hsS, u, AF.Gelu)
                # tf branch (off critical path): w = tf - ctx
                w_t = smr.tile([128, KD, B], F32, tag="w_t")
                if t > 0:
                    tfp = prc.tile([128, KD, B], F32, tag="ptf")
                    for m in range(KD):
                        for k in range(KD):
                            nc.tensor.matmul(tfp[:, m, :], Wtf[:, k, m * 128:(m + 1) * 128],
                                             hprev[:, k, :], start=(k == 0),
                                             stop=(k == KD - 1))
                    nc.vector.tensor_sub(w_t, tfp, ctx_t)
                else:
                    nc.vector.tensor_scalar_mul(w_t, ctx_t, -1.0)
                cpr = pr6.tile([128, KD, B], F32, tag="p6")
                for m in range(KD):
                    for k in range(MH):
                        nc.tensor.matmul(cpr[:, m, :], V1w[:, k, m * 128:(m + 1) * 128],
                                         hsS[:, k, :], start=(k == 0), stop=False)
                    nc.tensor.matmul(cpr[:, m, :], eye, b1b[:, m, :],
                                     start=False, stop=True)
                # cpf stored b-major so per-b layernorm views are contiguous
                # (gpsimd firmware requires contiguous [128, F] rows)
                cpf = smr.tile([128, B, KD], F32, tag="cpf")
                nc.scalar.activation(cpf[:, :, :].rearrange("p b k -> p k b"),
                                     cpr, AF.Gelu)
                # l2norm via gpsimd rmsnorm (no act-table); 1/sqrt(D) folded
                # into cw1 host-side.
                corePool = smr.tile([128, B, KD], F32, tag="corePool")
                for b in range(B):
                    nc.gpsimd.layernorm(corePool[:, b, :], cpf[:, b, :],
                                        subtract_mean=False, eps=1e-10)
                coreS = smr.tile([128, KD, B], BF, tag="coreS")
                nc.vector.tensor_copy(coreS,
                                      corePool[:, :, :].rearrange("p b k -> p k b"))
                gm = pr6.tile([128, KD, B], F32, tag="p6")
                for m in range(KD):
                    for k in range(KD):
                        nc.tensor.matmul(gm[:, m, :], cw1[:, k, m * 128:(m + 1) * 128],
                                         coreS[:, k, :], start=(k == 0), stop=False)
                    nc.tensor.matmul(gm[:, m, :], eye, cb1b[:, m, :],
                                     start=False, stop=True)
                gmS = smr.tile([128, KD, B], BF, tag="gmS")
                nc.scalar.activation(gmS, gm, AF.Gelu)
                cfp = pr6.tile([128, KD, B], F32, tag="p6")
                for m in range(KD):
                    for k in range(KD):
                        nc.tensor.matmul(cfp[:, m, :], cw2[:, k, m * 128:(m + 1) * 128],
                                         gmS[:, k, :], start=(k == 0), stop=False)
                    nc.tensor.matmul(cfp[:, m, :], eye, cb2b[:, m, :],
                                     start=False, stop=True)
                cfF = smr.tile([128, KD, B], F32, tag="cfF")
                nc.scalar.activation(cfF, cfp, AF.Copy)
                # gate cf-branch folded through cp_w2: reads g1 (gmS) directly,
                # in parallel with the cfp/cfF branch
                gp = pr6.tile([128, KD, B], F32, tag="pgp")
                for m in range(KD):
                    nc.tensor.matmul(gp[:, m, :], eye, gbb[:, m, :],
                                     start=True, stop=False)
                    for k in range(KD):
                        nc.tensor.matmul(gp[:, m, :], gw1w[:, k, m * 128:(m + 1) * 128],
                                         ctx_t[:, k, :], start=False, stop=False)
                    for k in range(KD):
                        nc.tensor.matmul(gp[:, m, :], gw2[:, k, m * 128:(m + 1) * 128],
                                         gmS[:, k, :], start=False,
                                         stop=(k == KD - 1))
                # gate via tanh (shares the gelu act table):
                #   sigmoid(x) = 0.5*(1 + tanh(x/2))
                #   hp = gate*(cf+tf-ctx) + ctx = 0.5*(tanh+1)*(cf+tf-ctx) + ctx
                tG = smr.tile([128, KD, B], F32, tag="gateS")
                nc.scalar.activation(tG, gp, AF.Tanh, scale=0.5)
                a1 = smr.tile([128, KD, B], F32, tag="a1")
                nc.vector.tensor_add(a1, cfF, w_t)
                q1 = smr.tile([128, KD, B], F32, tag="q1")
                nc.vector.scalar_tensor_tensor(q1, tG, 1.0, a1, OP.add, OP.mult)
                hp = smr.tile([128, B, KD], F32, tag="hp")
                nc.vector.scalar_tensor_tensor(
                    hp[:, :, :].rearrange("p b k -> p k b"),
                    q1, 0.5, ctx_t, OP.mult, OP.add)
                # LN + affine via gpsimd layernorm (no act-table), then clip.
                lnout = smr.tile([128, B, KD], F32, tag="lnout")
                for b in range(B):
                    nc.gpsimd.layernorm(lnout[:, b, :], hp[:, b, :],
                                        gamma_ap=outgS[:, :], beta_ap=outbS[:, :],
                                        subtract_mean=True, eps=1e-5)
                nc.vector.tensor_scalar(Hc[t // 8][:, :, (t % 8) * B:(t % 8) * B + B],
                                        lnout[:, :, :].rearrange("p b k -> p k b"),
                                        5.0, -5.0, OP.min, OP.max)
                if KDEBUG and t == min(7, T_RUN - 1):
                    nc.sync.dma_start(out=dbg_h[:, :],
                                      in_=Hc[0][:, :, :].rearrange("p k c -> p (k c)"))
                if KDEBUG and t == int(_os.environ.get("KPROBE_T", 0)):
                    for nm, tile_ in [("hs", hsS), ("cpf", cpf), ("core", coreS),
                                      ("gm", gmS), ("cf", cfF), ("tg", tG),
                                      ("hp", hp), ("ln", lnout)]:
                        nc.sync.dma_start(
                            out=dbg_st[nm][:, :],
                            in_=tile_[:, :, :].rearrange("p a b -> p (a b)"))
                # interleave lm-head work for the previous (complete) block
                if t >= 8 and T_RUN == T:
                    lm_chunks(mtb - 1, LM_SPLIT[t % 8], LM_SPLIT[t % 8 + 1])

            # lm-head tail: last block (or all blocks on truncated debug runs)
            tail_blocks = range(NBLK - 1, NBLK) if T_RUN == T else range(NBLK)
            for mt in tail_blocks:
                lm_chunks(mt, 0, NCH)

    nc.finalize()
    return nc


_NC_CACHE = {}


def _get_nc():
    key = (_FLAGS["ln_triv"], _FLAGS["bv0"])
    if key not in _NC_CACHE:
        _NC_CACHE[key] = build_nc()
    return _NC_CACHE[key]


def _prep_inputs(inputs):
    f = lambda x: np.asarray(x, np.float32)
    tok = np.asarray(inputs["token_ids"]).astype(np.int64)
    emb, pos = f(inputs["emb"]), f(inputs["pos_emb"])
    x0 = emb[tok.reshape(-1)] + np.tile(pos[:T], (B, 1))
    com = {"x0T": x0.T.astype(bfnp)}
    aiw, aib = f(inputs["attn_in_w"]), f(inputs["attn_in_b"])
    aow, aob = f(inputs["attn_out_w"]), f(inputs["attn_out_b"])
    for l in range(NL):
        wqk = np.zeros((D, 2048), np.float32)
        bqk = np.zeros(2048, np.float32)
        wv = np.zeros((D, 1024), np.float32)
        bv = np.zeros(1024, np.float32)
        wao = np.zeros((1024, D), np.float32)
        for h in range(NH):
            wqk[:, h * 128:h * 128 + HD] = aiw[l, h * HD:(h + 1) * HD, :].T
            wqk[:, 1024 + h * 128:1024 + h * 128 + HD] = aiw[l, D + h * HD:D + (h + 1) * HD, :].T
            bqk[h * 128:h * 128 + HD] = aib[l, h * HD:(h + 1) * HD]
            bqk[1024 + h * 128:1024 + h * 128 + HD] = aib[l, D + h * HD:D + (h + 1) * HD]
            wv[:, h * 128:h * 128 + HD] = aiw[l, 2 * D + h * HD:2 * D + (h + 1) * HD, :].T
            bv[h * 128:h * 128 + HD] = aib[l, 2 * D + h * HD:2 * D + (h + 1) * HD]
            wao[h * 128:h * 128 + HD, :] = aow[l][:, h * HD:(h + 1) * HD].T
        com[f"wqk{l}"] = wqk.astype(bfnp)
        com[f"bqk{l}"] = bqk.reshape(16, 128).T.copy()
        com[f"wv{l}"] = wv.astype(bfnp)
        com[f"bv{l}"] = bv.reshape(1, 1024).astype(bfnp)
        com[f"wao{l}"] = wao.astype(bfnp)
        com[f"bao{l}"] = aob[l].reshape(KD, 128).T.copy()
        com[f"ff1T{l}"] = f(inputs["ff_w1"])[l].T.astype(bfnp).copy()
        com[f"bff1{l}"] = f(inputs["ff_b1"])[l].reshape(MF, 128).T.copy()
        com[f"ff2T{l}"] = f(inputs["ff_w2"])[l].T.astype(bfnp).copy()
        com[f"bff2{l}"] = f(inputs["ff_b2"])[l].reshape(KD, 128).T.copy()
        com[f"g1{l}"] = f(inputs["n1_g"])[l].reshape(KD, 128).T.copy()
        com[f"bb1{l}"] = f(inputs["n1_b"])[l].reshape(KD, 128).T.copy()
        com[f"g2{l}"] = f(inputs["n2_g"])[l].reshape(KD, 128).T.copy()
        com[f"bb2{l}"] = f(inputs["n2_b"])[l].reshape(KD, 128).T.copy()
    com["encg"] = f(inputs["enc_norm_g"]).reshape(KD, 128).T.copy()
    com["encb"] = f(inputs["enc_norm_b"]).reshape(KD, 128).T.copy()
    tk, tq = np.meshgrid(np.arange(64), np.arange(64), indexing="ij")
    com["maskT"] = ((tk > tq) * -30000.0).astype(bfnp)
    com["eyed"] = np.eye(128, dtype=bfnp)
    com["V0d"] = f(inputs["V0"]).astype(bfnp)
    com["V1d"] = f(inputs["V1"]).astype(bfnp)
    R, tw = f(inputs["R"]), f(inputs["temp_w"])
    # folded recurrent weights: u += (alpha*R@V0)^T h ; tf = (alpha*R@tw^T)^T h
    com["Whud"] = (ALPHA * R @ f(inputs["V0"])).astype(bfnp)
    com["Wtfd"] = (ALPHA * R @ tw.T).astype(bfnp)
    # rmsnorm(x) = sqrt(D) * l2norm(x): fold the 1/sqrt(D) into cp_w1
    com["cw1d"] = (f(inputs["cp_w1"]).T / np.sqrt(D)).astype(bfnp).copy()
    com["cw2d"] = f(inputs["cp_w2"]).T.astype(bfnp).copy()
    gw = f(inputs["gate_w"])
    com["gw1d"] = gw[:, :D].T.astype(bfnp).copy()
    # gate cf-branch folded through cp_w2: gs_cf = Wgg^T g1 (+ gwB@cp_b2 -> gb)
    gwB = gw[:, D:]
    com["gw2d"] = np.ascontiguousarray((gwB @ f(inputs["cp_w2"])).T).astype(bfnp)
    com["gbd"] = (f(inputs["gate_b"]) + gwB @ f(inputs["cp_b2"])).reshape(KD, 128).T.copy()
    com["b0d"] = f(inputs["b0"]).reshape(MH, 128).T.copy()
    com["b1d"] = f(inputs["b1"]).reshape(KD, 128).T.copy()
    com["cb1d"] = f(inputs["cp_b1"]).reshape(KD, 128).T.copy()
    com["cb2d"] = f(inputs["cp_b2"]).reshape(KD, 128).T.copy()
    com["outgd"] = np.ascontiguousarray(f(inputs["out_g"]).reshape(KD, 128).T)
    com["outbd"] = np.ascontiguousarray(f(inputs["out_b"]).reshape(KD, 128).T)
    lmp = np.zeros((VP, D), np.float32)
    lmp[:V] = f(inputs["lm_head"])
    lmT = lmp.T.astype(bfnp)
    shards = [np.ascontiguousarray(lmT[:, c * VS:(c + 1) * VS]) for c in range(8)]
    return com, shards


LAST_RESULT = {}


def kernel(**inputs):
    import os
    f = lambda x: np.asarray(x, np.float32)
    _FLAGS["ln_triv"] = bool(
        all(np.all(f(inputs[k]) == 1.0) for k in ("n1_g", "n2_g", "enc_norm_g"))
        and all(np.all(f(inputs[k]) == 0.0) for k in ("n1_b", "n2_b", "enc_norm_b")))
    _FLAGS["bv0"] = bool(np.all(f(inputs["attn_in_b"])[:, 2 * D:] == 0.0))
    nc = _get_nc()
    com, shards = _prep_inputs(inputs)
    in_maps = [{**com, "lmTd": shards[c]} for c in range(8)]
    kw = {}
    if os.environ.get("KTRACE"):
        kw = dict(trace=True, tmpdir=os.environ.get("KTRACE_DIR", "/root/problem/trace_out"))
    res = run_bass_kernel_spmd(nc, in_maps, core_ids=list(range(8)), **kw)
    LAST_RESULT["res"] = res
    parts = [res.results[c]["out"] for c in range(8)]          # each [1024, VS], t-major rows
    full = np.concatenate(parts, axis=1)[:, :V].astype(np.float32)
    return np.ascontiguousarray(full.reshape(T, B, V).transpose(1, 0, 2))



# revision 59
# speedup vs baseline: 1.4907x; 1.0529x over previous
"""Trainium2 Bass kernel for nn_AgnisV5: 2-layer GPT encoder + gated
hierarchical recurrence + lm_head, SPMD over 8 NeuronCores.

Strategy: encoder + recurrence replicated on all cores (no collectives);
lm_head vocab-sharded 8 ways. Forward-pass simplification: stop_gradient is
identity, so the "settled" hierarchy path equals the differentiable shadow
path and blend == core_sh.

Layouts: feature-major activations [128p, K, N] (feature f = k*128 + p).
Encoder token columns are b-major (n = b*64 + t); recurrence/H/lm_head use
t-major (n = t*16 + b) via strided views of ctx.
"""
import numpy as np
import ml_dtypes
from contextlib import ExitStack

import concourse.bass as bass
import concourse.mybir as mybir
import concourse.tile as tile
from concourse import bacc
from concourse.bass_utils import run_bass_kernel_spmd

AF = mybir.ActivationFunctionType
OP = mybir.AluOpType
BF = mybir.dt.float16
F32 = mybir.dt.float32

D, HID, FFN, NH, NL, T, V, B = 768, 3072, 2048, 8, 2, 64, 50257, 16
HD = 96
KD = D // 128          # 6
MH = HID // 128        # 24
MF = FFN // 128        # 16
NTOK = B * T           # 1024
MT = NTOK // 128       # 8
VP = 53248             # padded vocab = 8*6656
VS = VP // 8           # 6656 per core
NCH = VS // 512        # 13
ALPHA = 0.4
QK_SCALE = 1.0 / np.sqrt(96.0)

bfnp = np.float16


_FLAGS = {"ln_triv": False, "bv0": False}


def bc(ap, reps, pos):
    """Insert a stride-0 (broadcast) free dim at position pos (1-based over free dims)."""
    newap = list(ap.ap)
    newap.insert(pos, [0, reps])
    return bass.AP(tensor=ap.tensor, offset=ap.offset, ap=newap)


def build_nc():
    LN_TRIV = _FLAGS["ln_triv"]
    BV0 = _FLAGS["bv0"]
    nc = bacc.Bacc(None, target_bir_lowering=False)

    # ---- DRAM I/O ----
    x0T = nc.dram_tensor("x0T", [D, NTOK], BF, kind="ExternalInput")
    enc_in = {}
    for l in range(NL):
        enc_in[f"wqk{l}"] = nc.dram_tensor(f"wqk{l}", [D, 2048], BF, kind="ExternalInput")
        enc_in[f"wv{l}"] = nc.dram_tensor(f"wv{l}", [D, 1024], BF, kind="ExternalInput")
        enc_in[f"wao{l}"] = nc.dram_tensor(f"wao{l}", [1024, D], BF, kind="ExternalInput")
        enc_in[f"ff1T{l}"] = nc.dram_tensor(f"ff1T{l}", [D, FFN], BF, kind="ExternalInput")
        enc_in[f"ff2T{l}"] = nc.dram_tensor(f"ff2T{l}", [FFN, D], BF, kind="ExternalInput")
        enc_in[f"bqk{l}"] = nc.dram_tensor(f"bqk{l}", [128, 16], F32, kind="ExternalInput")
        enc_in[f"bv{l}"] = nc.dram_tensor(f"bv{l}", [1, 1024], BF, kind="ExternalInput")
        enc_in[f"bao{l}"] = nc.dram_tensor(f"bao{l}", [128, KD], F32, kind="ExternalInput")
        enc_in[f"bff1{l}"] = nc.dram_tensor(f"bff1{l}", [128, MF], F32, kind="ExternalInput")
        enc_in[f"bff2{l}"] = nc.dram_tensor(f"bff2{l}", [128, KD], F32, kind="ExternalInput")
        enc_in[f"g1{l}"] = nc.dram_tensor(f"g1{l}", [128, KD], F32, kind="ExternalInput")
        enc_in[f"bb1{l}"] = nc.dram_tensor(f"bb1{l}", [128, KD], F32, kind="ExternalInput")
        enc_in[f"g2{l}"] = nc.dram_tensor(f"g2{l}", [128, KD], F32, kind="ExternalInput")
        enc_in[f"bb2{l}"] = nc.dram_tensor(f"bb2{l}", [128, KD], F32, kind="ExternalInput")
    encg = nc.dram_tensor("encg", [128, KD], F32, kind="ExternalInput")
    encb = nc.dram_tensor("encb", [128, KD], F32, kind="ExternalInput")
    maskT = nc.dram_tensor("maskT", [64, 64], BF, kind="ExternalInput")
    V0d = nc.dram_tensor("V0d", [D, HID], BF, kind="ExternalInput")
    V1d = nc.dram_tensor("V1d", [HID, D], BF, kind="ExternalInput")
    Whud = nc.dram_tensor("Whud", [D, HID], BF, kind="ExternalInput")
    Wtfd = nc.dram_tensor("Wtfd", [D, D], BF, kind="ExternalInput")
    cw1d = nc.dram_tensor("cw1d", [D, D], BF, kind="ExternalInput")
    cw2d = nc.dram_tensor("cw2d", [D, D], BF, kind="ExternalInput")
    gw1d = nc.dram_tensor("gw1d", [D, D], BF, kind="ExternalInput")
    gw2d = nc.dram_tensor("gw2d", [D, D], BF, kind="ExternalInput")
    b0d = nc.dram_tensor("b0d", [128, MH], F32, kind="ExternalInput")
    b1d = nc.dram_tensor("b1d", [128, KD], F32, kind="ExternalInput")
    cb1d = nc.dram_tensor("cb1d", [128, KD], F32, kind="ExternalInput")
    cb2d = nc.dram_tensor("cb2d", [128, KD], F32, kind="ExternalInput")
    gbd = nc.dram_tensor("gbd", [128, KD], F32, kind="ExternalInput")
    outgd = nc.dram_tensor("outgd", [128, KD], F32, kind="ExternalInput")
    outbd = nc.dram_tensor("outbd", [128, KD], F32, kind="ExternalInput")
    lmTd = nc.dram_tensor("lmTd", [D, VS], BF, kind="ExternalInput")
    eyed = nc.dram_tensor("eyed", [128, 128], BF, kind="ExternalInput")
    outd = nc.dram_tensor("out", [NTOK, VS], BF, kind="ExternalOutput")
    import os as _os0
    KDEBUG = bool(_os0.environ.get("KDEBUG"))
    if KDEBUG:
        dbg_ctx = nc.dram_tensor("dbg_ctx", [128, KD * NTOK], BF, kind="ExternalOutput")
        dbg_uc = nc.dram_tensor("dbg_uc", [128, MH * 128], BF, kind="ExternalOutput")
        dbg_h = nc.dram_tensor("dbg_h", [128, KD * 128], BF, kind="ExternalOutput")
        dbg_st = {}
        for nm, width, dt_ in [("hs", MH * B, BF), ("cpf", KD * B, F32),
                               ("core", KD * B, BF), ("gm", KD * B, BF),
                               ("cf", KD * B, F32), ("tg", KD * B, F32),
                               ("hp", KD * B, F32), ("ln", KD * B, F32)]:
            dbg_st[nm] = nc.dram_tensor(f"dbg_{nm}", [128, width], dt_,
                                        kind="ExternalOutput")

    def r3(h, p=128):
        return h[:, :].rearrange("(k p) n -> p k n", p=p)

    with ExitStack() as ctx:
        tc = ctx.enter_context(tile.TileContext(nc))
        cross = ctx.enter_context(tc.tile_pool(name="cross", bufs=1))

        # constants (cross-scope)
        ones_col = cross.tile([128, 1], BF, tag="ones_col")
        nc.vector.memset(ones_col, 1.0)
        ones64 = cross.tile([64, 1], BF, tag="ones64")
        nc.vector.memset(ones64, 1.0)
        ones_row_f = cross.tile([1, 128], F32, tag="ones_row_f")
        nc.vector.memset(ones_row_f, 1.0)
        ones_r64f = cross.tile([1, 64], F32, tag="ones_r64f")
        nc.vector.memset(ones_r64f, 1.0)
        mask_sb = cross.tile([64, 64], BF, tag="mask")
        nc.sync.dma_start(out=mask_sb, in_=maskT[:, :])
        eye = cross.tile([128, 128], BF, tag="eye")
        nc.sync.dma_start(out=eye, in_=eyed[:, :])
        eps5 = cross.tile([128, 1], F32, tag="eps5")
        nc.vector.memset(eps5, 1e-5)
        eps24 = cross.tile([128, 1], F32, tag="eps24")
        nc.vector.memset(eps24, 1e-24)
        ctxF = cross.tile([128, KD, NTOK], BF, tag="ctxF")

        # ================= ENCODER SCOPE =================
        with tc.tile_pool(name="wenc", bufs=1) as wenc, \
             tc.tile_pool(name="bige", bufs=1) as bige, \
             tc.tile_pool(name="sme", bufs=1) as sme, \
             tc.tile_pool(name="pmm", bufs=2, space="PSUM") as pmm, \
             tc.tile_pool(name="pst", bufs=1, space="PSUM") as pst, \
             tc.tile_pool(name="pbc", bufs=2, space="PSUM") as pbc:

            def ln_fm(x_sb, g_ap, b_ap, out, l2_out=None, triv=False):
                """x_sb [128, KD, 1024] bf16 -> LN -> out; optional l2 -> l2_out.
                triv: gamma==1 and beta==0 (runtime-specialized)."""
                for j in range(2):
                    jj = slice(j * 512, (j + 1) * 512)
                    s12 = pst.tile([1, 2, 512], F32, tag="s1")
                    for k in range(KD):
                        sq = bige.tile([128, 512], BF, tag="sqc", bufs=2)
                        nc.vector.tensor_mul(sq, x_sb[:, k, jj], x_sb[:, k, jj])
                        nc.tensor.matmul(s12[:, 0, :], ones_col, x_sb[:, k, jj],
                                         start=(k == 0), stop=(k == KD - 1))
                        nc.tensor.matmul(s12[:, 1, :], ones_col, sq,
                                         start=(k == 0), stop=(k == KD - 1))
                    m = sme.tile([1, 512], F32, tag="mtag", bufs=2)
                    nc.vector.tensor_scalar_mul(m, s12[:, 0, :], 1.0 / D)
                    msq = sme.tile([1, 512], F32, tag="scr512", bufs=2)
                    nc.vector.tensor_mul(msq, m, m)
                    var = sme.tile([1, 512], F32, tag="scr512", bufs=2)
                    nc.vector.scalar_tensor_tensor(var, s12[:, 1, :], 1.0 / D,
                                                   msq, OP.mult, OP.subtract)
                    sd = sme.tile([1, 512], F32, tag="scr512", bufs=2)
                    nc.scalar.activation(sd, var, AF.Sqrt, bias=eps5[:1, :])
                    ac = sme.tile([1, 2, 512], F32, tag="acf")
                    nc.vector.reciprocal(ac[:, 0, :], sd)
                    nc.vector.scalar_tensor_tensor(ac[:, 1, :], m, -1.0,
                                                   ac[:, 0, :], OP.mult, OP.mult)
                    pac = pbc.tile([128, 2, 512], F32, tag="pabc", bufs=1)
                    for a in range(2):
                        nc.tensor.matmul(pac[:, a, :], ones_row_f, ac[:, a, :],
                                         start=True, stop=True)
                    pab = bige.tile([128, 2, 512], BF, tag="pab16", bufs=2)
                    nc.scalar.activation(pab, pac, AF.Copy)
                    for k in range(KD):
                        if triv:
                            t1 = bige.tile([128, 512], BF, tag="t1", bufs=2)
                            nc.vector.tensor_mul(t1, x_sb[:, k, jj], pab[:, 0, :])
                            nc.vector.tensor_add(out[:, k, jj], t1, pab[:, 1, :])
                        else:
                            t1 = bige.tile([128, 512], F32, tag="t1f", bufs=2)
                            nc.vector.tensor_mul(t1, x_sb[:, k, jj], pab[:, 0, :])
                            nc.vector.tensor_add(t1, t1, pab[:, 1, :])
                            nc.vector.tensor_scalar(out[:, k, jj], t1,
                                                    g_ap[:, k:k + 1],
                                                    b_ap[:, k:k + 1],
                                                    OP.mult, OP.add)
                if l2_out is None:
                    return
                for j in range(2):
                    jj = slice(j * 512, (j + 1) * 512)
                    s1t = pst.tile([1, 2, 512], F32, tag="s1")
                    s1 = s1t[:, 0, :]
                    for k in range(KD):
                        sq = bige.tile([128, 512], BF, tag="sqc", bufs=2)
                        nc.vector.tensor_mul(sq, out[:, k, jj], out[:, k, jj])
                        nc.tensor.matmul(s1, ones_col, sq,
                                         start=(k == 0), stop=(k == KD - 1))
                    sd = sme.tile([1, 512], F32, tag="scr512", bufs=2)
                    nc.scalar.activation(sd, s1, AF.Sqrt, bias=eps24[:1, :])
                    rr = sme.tile([1, 512], F32, tag="scr512", bufs=2)
                    nc.vector.reciprocal(rr, sd)
                    pat = pbc.tile([128, 2, 512], F32, tag="pabc", bufs=1)
                    pa = pat[:, 0, :]
                    nc.tensor.matmul(pa, ones_row_f, rr, start=True, stop=True)
                    pa16 = bige.tile([128, 512], BF, tag="pa16", bufs=2)
                    nc.scalar.activation(pa16, pa, AF.Copy)
                    for k in range(KD):
                        nc.vector.tensor_mul(l2_out[:, k, jj], out[:, k, jj], pa16)

            X = bige.tile([128, KD, NTOK], BF, tag="X")
            nc.sync.dma_start(out=X, in_=r3(x0T))
            gA = sme.tile([128, NL, KD], F32, tag="gA")
            bA = sme.tile([128, NL, KD], F32, tag="bA")
            gB = sme.tile([128, NL, KD], F32, tag="gB")
            bB = sme.tile([128, NL, KD], F32, tag="bB")
            for l in range(NL):
                nc.sync.dma_start(out=gA[:, l, :], in_=enc_in[f"g1{l}"][:, :])
                nc.sync.dma_start(out=bA[:, l, :], in_=enc_in[f"bb1{l}"][:, :])
                nc.sync.dma_start(out=gB[:, l, :], in_=enc_in[f"g2{l}"][:, :])
                nc.sync.dma_start(out=bB[:, l, :], in_=enc_in[f"bb2{l}"][:, :])

            for l in range(NL):
                XN = bige.tile([128, KD, NTOK], BF, tag="XN")
                ln_fm(X, gA[:, l, :], bA[:, l, :], XN, triv=LN_TRIV)
                wqk = wenc.tile([128, KD, 2048], BF, tag="wbig")
                nc.sync.dma_start(out=wqk, in_=r3(enc_in[f"wqk{l}"]))
                bqk = sme.tile([128, 16], F32, tag="bqk")
                nc.sync.dma_start(out=bqk, in_=enc_in[f"bqk{l}"][:, :])
                # v projection (token-major)
                wv = wenc.tile([128, KD, 1024], BF, tag="wsmall")
                nc.sync.dma_start(out=wv, in_=r3(enc_in[f"wv{l}"]))
                if not BV0:
                    ones_row = sme.tile([1, 128], BF, tag="ones_row")
                    nc.vector.memset(ones_row, 1.0)
                    bvr = sme.tile([1, 1024], BF, tag="bvr")
                    nc.sync.dma_start(out=bvr, in_=enc_in[f"bv{l}"][:, :])
                    bvs = bige.tile([128, 1024], BF, tag="bvs")
                    for j in range(2):
                        pb = pbc.tile([128, 2, 512], F32, tag="pabc", bufs=1)
                        nc.tensor.matmul(pb[:, 0, :], ones_row,
                                         bvr[:, j * 512:(j + 1) * 512],
                                         start=True, stop=True)
                        nc.vector.tensor_copy(bvs[:, j * 512:(j + 1) * 512],
                                              pb[:, 0, :])
                vS = bige.tile([64, B, 1024], BF, tag="vS")
                for b in range(B):
                    ps = pmm.tile([64, 1024], F32, tag="pmm")
                    for k in range(KD):
                        for j in range(2):
                            nc.tensor.matmul(ps[:, j * 512:(j + 1) * 512],
                                             XN[:, k, b * 64:(b + 1) * 64],
                                             wv[:, k, j * 512:(j + 1) * 512],
                                             start=(k == 0), stop=(k == KD - 1))
                    if not BV0:
                        nc.vector.tensor_add(vS[:, b, :], ps, bvs[:64, :])
                    elif b % 2 == 0:
                        nc.scalar.activation(vS[:, b, :], ps, AF.Copy)
                    else:
                        nc.vector.tensor_copy(vS[:, b, :], ps)
                # attention per head; additive mask injected into score PSUM
                ofS = bige.tile([128, NH, NTOK], BF, tag="ofS")
                for h in range(NH):
                    qh = bige.tile([128, NTOK], BF, tag="qh", bufs=1)
                    ph = pmm.tile([128, 1024], F32, tag="pmm")
                    for k in range(KD):
                        for j in range(2):
                            nc.tensor.matmul(ph[:, j * 512:(j + 1) * 512],
                                             wqk[:, k, h * 128:(h + 1) * 128],
                                             XN[:, k, j * 512:(j + 1) * 512],
                                             start=(k == 0), stop=(k == KD - 1))
                    nc.vector.tensor_scalar_add(qh, ph, bqk[:, h:h + 1])
                    kh = bige.tile([128, NTOK], BF, tag="kh", bufs=1)
                    ph2 = pmm.tile([128, 1024], F32, tag="pmm")
                    for k in range(KD):
                        for j in range(2):
                            nc.tensor.matmul(ph2[:, j * 512:(j + 1) * 512],
                                             wqk[:, k, 1024 + h * 128:1024 + (h + 1) * 128],
                                             XN[:, k, j * 512:(j + 1) * 512],
                                             start=(k == 0), stop=(k == KD - 1))
                    nc.scalar.activation(kh, ph2, AF.Identity,
                                         bias=bqk[:, 8 + h:9 + h])
                    sT = pmm.tile([64, 1024], F32, tag="pmm")
                    for b in range(B):
                        nc.tensor.matmul(sT[:, b * 64:(b + 1) * 64],
                                         kh[:, b * 64:(b + 1) * 64],
                                         qh[:, b * 64:(b + 1) * 64],
                                         start=True, stop=False)
                        nc.tensor.matmul(sT[:, b * 64:(b + 1) * 64],
                                         eye[:64, :64], mask_sb,
                                         start=False, stop=True)
                    eT = bige.tile([64, NTOK], BF, tag="eT", bufs=2)
                    nc.scalar.activation(eT, sT, AF.Exp, scale=QK_SCALE)
                    ssum = pst.tile([1, 2, 512], F32, tag="s1")
                    for a in range(2):
                        nc.tensor.matmul(ssum[:, a, :], ones64,
                                         eT[:, a * 512:(a + 1) * 512],
                                         start=True, stop=True)
                    rs2 = sme.tile([1, 2, 512], F32, tag="acf")
                    rs = rs2[:, :, :].rearrange("p a c -> p (a c)")
                    nc.vector.reciprocal(rs, ssum[:, :, :].rearrange("p a c -> p (a c)"))
                    rb64 = pmm.tile([128, 1024], F32, tag="pmm")
                    for a in range(2):
                        nc.tensor.matmul(rb64[:64, a * 512:(a + 1) * 512],
                                         ones_r64f, rs[:, a * 512:(a + 1) * 512],
                                         start=True, stop=True)
                    eN = bige.tile([64, NTOK], BF, tag="eN", bufs=2)
                    nc.vector.tensor_mul(eN, eT, rb64[:64, :])
                    oh = pmm.tile([128, 1024], F32, tag="pmm")
                    for b in range(B):
                        nc.tensor.matmul(oh[:, b * 64:(b + 1) * 64],
                                         vS[:, b, h * 128:(h + 1) * 128],
                                         eN[:, b * 64:(b + 1) * 64],
                                         start=True, stop=True)
                    if h % 2 == 0:
                        nc.scalar.activation(ofS[:, h, :], oh, AF.Copy)
                    else:
                        nc.vector.tensor_copy(ofS[:, h, :], oh)
                # attn out + residual (in place on X)
                wao = wenc.tile([128, NH, D], BF, tag="wsmall")
                nc.sync.dma_start(out=wao,
                                  in_=enc_in[f"wao{l}"][:, :].rearrange("(k p) n -> p k n", p=128))
                bao = sme.tile([128, KD], F32, tag="bao")
                nc.sync.dma_start(out=bao, in_=enc_in[f"bao{l}"][:, :])
                for mo in range(KD):
                    ps = pmm.tile([128, 1024], F32, tag="pmm")
                    for kv in range(NH):
                        for j in range(2):
                            nc.tensor.matmul(ps[:, j * 512:(j + 1) * 512],
                                             wao[:, kv, mo * 128:(mo + 1) * 128],
                                             ofS[:, kv, j * 512:(j + 1) * 512],
                                             start=(kv == 0), stop=(kv == NH - 1))
                    t2 = bige.tile([128, 1024], F32, tag="t2", bufs=2)
                    nc.vector.tensor_scalar_add(t2, ps, bao[:, mo:mo + 1])
                    nc.vector.tensor_add(X[:, mo, :], X[:, mo, :], t2)
                # ffn
                XN2 = bige.tile([128, KD, NTOK], BF, tag="XN")
                ln_fm(X, gB[:, l, :], bB[:, l, :], XN2, triv=LN_TRIV)
                w1 = wenc.tile([128, KD, FFN], BF, tag="wbig")
                nc.sync.dma_start(out=w1, in_=r3(enc_in[f"ff1T{l}"]))
                bf1 = sme.tile([128, MF], F32, tag="bf1")
                nc.sync.dma_start(out=bf1, in_=enc_in[f"bff1{l}"][:, :])
                G = bige.tile([128, MF, NTOK], BF, tag="G")
                for mf in range(MF):
                    ps = pmm.tile([128, 1024], F32, tag="pmm")
                    for k in range(KD):
                        for j in range(2):
                            nc.tensor.matmul(ps[:, j * 512:(j + 1) * 512],
                                             w1[:, k, mf * 128:(mf + 1) * 128],
                                             XN2[:, k, j * 512:(j + 1) * 512],
                                             start=(k == 0), stop=(k == KD - 1))
                    nc.scalar.activation(G[:, mf, :], ps, AF.Gelu, bias=bf1[:, mf:mf + 1])
                w2 = wenc.tile([128, MF, D], BF, tag="wbig")
                nc.sync.dma_start(out=w2, in_=r3(enc_in[f"ff2T{l}"]))
                bf2 = sme.tile([128, KD], F32, tag="bao")
                nc.sync.dma_start(out=bf2, in_=enc_in[f"bff2{l}"][:, :])
                for mo in range(KD):
                    ps = pmm.tile([128, 1024], F32, tag="pmm")
                    for kf in range(MF):
                        for j in range(2):
                            nc.tensor.matmul(ps[:, j * 512:(j + 1) * 512],
                                             w2[:, kf, mo * 128:(mo + 1) * 128],
                                             G[:, kf, j * 512:(j + 1) * 512],
                                             start=(kf == 0), stop=(kf == MF - 1))
                    t2 = bige.tile([128, 1024], F32, tag="t2", bufs=2)
                    nc.vector.tensor_scalar_add(t2, ps, bf2[:, mo:mo + 1])
                    nc.vector.tensor_add(X[:, mo, :], X[:, mo, :], t2)

            # final norm + l2 -> ctxF (cross pool)
            eg = sme.tile([128, KD], F32, tag="eg")
            eb = sme.tile([128, KD], F32, tag="eb")
            nc.sync.dma_start(out=eg, in_=encg[:, :])
            nc.sync.dma_start(out=eb, in_=encb[:, :])
            pre = bige.tile([128, KD, NTOK], BF, tag="XN")
            ln_fm(X, eg, eb, pre, l2_out=ctxF, triv=LN_TRIV)


        # ================= RECURRENCE SCOPE =================
        ctx_r = ctxF[:, :, :].rearrange("p k (b t) -> p t k b", b=B)
        with tc.tile_pool(name="wrec", bufs=1) as wrec, \
             tc.tile_pool(name="smr", bufs=2) as smr, \
             tc.tile_pool(name="prc", bufs=1, space="PSUM") as prc, \
             tc.tile_pool(name="pr6", bufs=1, space="PSUM") as pr6, \
             tc.tile_pool(name="prs", bufs=1, space="PSUM") as prs, \
             tc.tile_pool(name="pou", bufs=1, space="PSUM") as pou:

            Whu = wrec.tile([128, KD, HID], BF, tag="Whu")
            nc.sync.dma_start(out=Whu, in_=r3(Whud))
            V0w = wrec.tile([128, KD, HID], BF, tag="V0w")
            nc.sync.dma_start(out=V0w, in_=r3(V0d))
            gw1w = wrec.tile([128, KD, D], BF, tag="gw1w")
            nc.sync.dma_start(out=gw1w, in_=r3(gw1d))
            V1w = wrec.tile([128, MH, D], BF, tag="V1w")
            nc.sync.dma_start(out=V1w, in_=r3(V1d))
            Wtf = wrec.tile([128, KD, D], BF, tag="Wtf")
            nc.sync.dma_start(out=Wtf, in_=r3(Wtfd))
            cw1 = wrec.tile([128, KD, D], BF, tag="cw1")
            nc.sync.dma_start(out=cw1, in_=r3(cw1d))
            cw2 = wrec.tile([128, KD, D], BF, tag="cw2")
            nc.sync.dma_start(out=cw2, in_=r3(cw2d))
            gw2 = wrec.tile([128, KD, D], BF, tag="gw2")
            nc.sync.dma_start(out=gw2, in_=r3(gw2d))
            b1S = smr.tile([128, KD], F32, tag="b1S", bufs=1)
            nc.sync.dma_start(out=b1S, in_=b1d[:, :])
            cb1S = smr.tile([128, KD], F32, tag="cb1S", bufs=1)
            nc.sync.dma_start(out=cb1S, in_=cb1d[:, :])
            cb2S = smr.tile([128, KD], F32, tag="cb2S", bufs=1)
            nc.sync.dma_start(out=cb2S, in_=cb2d[:, :])
            # bf16 broadcast copies (B cols) for PSUM bias injection
            b0S = smr.tile([128, MH], F32, tag="b0S", bufs=1)
            nc.sync.dma_start(out=b0S, in_=b0d[:, :])
            b0b = smr.tile([128, MH, B], BF, tag="b0b", bufs=1)
            nc.vector.tensor_copy(b0b, bc(b0S[:, :], B, 2))
            gbS = smr.tile([128, KD], F32, tag="gbS", bufs=1)
            nc.sync.dma_start(out=gbS, in_=gbd[:, :])
            gbb = smr.tile([128, KD, B], BF, tag="gbb", bufs=1)
            nc.vector.tensor_copy(gbb, bc(gbS[:, :], B, 2))
            b1b = smr.tile([128, KD, B], BF, tag="b1b", bufs=1)
            nc.vector.tensor_copy(b1b, bc(b1S[:, :], B, 2))
            cb1b = smr.tile([128, KD, B], BF, tag="cb1b", bufs=1)
            nc.vector.tensor_copy(cb1b, bc(cb1S[:, :], B, 2))
            cb2b = smr.tile([128, KD, B], BF, tag="cb2b", bufs=1)
            nc.vector.tensor_copy(cb2b, bc(cb2S[:, :], B, 2))
            outgS = smr.tile([128, KD], F32, tag="outgS", bufs=1)
            nc.sync.dma_start(out=outgS, in_=outgd[:, :])
            outbS = smr.tile([128, KD], F32, tag="outbS", bufs=1)
            nc.sync.dma_start(out=outbS, in_=outbd[:, :])
            Hc = [wrec.tile([128, KD, 128], BF, tag=f"H{mt}", name=f"H{mt}")
                  for mt in range(MT)]

            import os as _os
            T_RUN = int(_os.environ.get("T_TRUNC", T))
            if T_RUN < T:
                for Hcm in Hc:
                    nc.vector.memset(Hcm, 0.0)

            NBLK = (T_RUN + 7) // 8
            if KDEBUG:
                nc.sync.dma_start(out=dbg_ctx[:, :],
                                  in_=ctxF[:, :, :].rearrange("p k n -> p (k n)"))

            # lm chunk emission: during step t of block w, process chunks of
            # block w-1 (Hc[w-1] is complete); 13 chunks spread over 8 steps.
            LM_SPLIT = [0, 2, 4, 6, 8, 10, 11, 12, 13]

            def lm_chunks(mt, j0, j1):
                for j in range(j0, j1):
                    lw = wrec.tile([128, KD, 512], BF, tag="lmw", bufs=2)
                    nc.sync.dma_start(
                        out=lw,
                        in_=lmTd[:, j * 512:(j + 1) * 512].rearrange("(k p) n -> p k n",
                                                                     p=128))
                    ps = pou.tile([128, 512], F32, tag="plm", bufs=2)
                    for k in range(KD):
                        nc.tensor.matmul(ps, Hc[mt][:, k, :], lw[:, k, :],
                                         start=(k == 0), stop=(k == KD - 1))
                    ot = smr.tile([128, 512], BF, tag="otile", bufs=3)
                    if j % 2 == 0:
                        nc.scalar.activation(ot, ps, AF.Copy)
                    else:
                        nc.vector.tensor_copy(ot, ps)
                    nc.sync.dma_start(out=outd[mt * 128:(mt + 1) * 128,
                                               j * 512:(j + 1) * 512], in_=ot)

            u_tiles = {}
            gp_tiles = {}

            def emit_u_prefix(tt):
                # b0 + V0^T ctx accumulation prefix for step tt (h-independent;
                # scheduler overlaps it with step tt-1's tail). One psum bank:
                # single start (zeroes whole 2KB region), single stop at the
                # group's true end.
                un = prc.tile([128, 32, B], F32, tag="pu", bufs=2,
                              name=f"u{tt}")[:, :MH, :]
                cx = ctx_r[:, tt]
                for m in range(MH):
                    nc.tensor.matmul(un[:, m, :], eye, b0b[:, m, :],
                                     start=(m == 0), stop=False)
                    for k in range(KD):
                        nc.tensor.matmul(un[:, m, :], V0w[:, k, m * 128:(m + 1) * 128],
                                         cx[:, k, :], start=False,
                                         stop=(tt == 0 and m == MH - 1 and k == KD - 1))
                u_tiles[tt] = un

            def emit_gp_prefix(tt):
                gpn = pr6.tile([128, 8, B], F32, tag="pgp", bufs=2,
                               name=f"gp{tt}")[:, :KD, :]
                cx = ctx_r[:, tt]
                for m in range(KD):
                    nc.tensor.matmul(gpn[:, m, :], eye, gbb[:, m, :],
                                     start=(m == 0), stop=False)
                    for k in range(KD):
                        nc.tensor.matmul(gpn[:, m, :],
                                         gw1w[:, k, m * 128:(m + 1) * 128],
                                         cx[:, k, :], start=False, stop=False)
                gp_tiles[tt] = gpn

            emit_u_prefix(0)
            emit_gp_prefix(0)
            for t in range(T_RUN):
                ctx_t = ctx_r[:, t]
                mtb, off = t // 8, (t % 8) * B
                if t > 0:
                    hprev = Hc[(t - 1) // 8][:, :, ((t - 1) % 8) * B:((t - 1) % 8) * B + B]
                u = u_tiles.pop(t)
                if t > 0:
                    for m in range(MH):
                        for k in range(KD):
                            nc.tensor.matmul(u[:, m, :], Whu[:, k, m * 128:(m + 1) * 128],
                                             hprev[:, k, :], start=False,
                                             stop=(m == MH - 1 and k == KD - 1))
                hsS = smr.tile([128, MH, B], BF, tag="hsS")
                nc.scalar.activation(hsS, u, AF.Gelu)
                # tf branch (off critical path): w = tf - ctx
                w_t = smr.tile([128, KD, B], F32, tag="w_t")
                if t > 0:
                    tfp = prc.tile([128, 8, B], F32, tag="ptf", name="tfp")[:, :KD, :]
                    for m in range(KD):
                        for k in range(KD):
                            nc.tensor.matmul(tfp[:, m, :], Wtf[:, k, m * 128:(m + 1) * 128],
                                             hprev[:, k, :], start=(k == 0),
                                             stop=(k == KD - 1))
                    nc.vector.tensor_sub(w_t, tfp, ctx_t)
                else:
                    nc.vector.tensor_scalar_mul(w_t, ctx_t, -1.0)
                cpr = pr6.tile([128, 8, B], F32, tag="p6", name="cpr")[:, :KD, :]
                for m in range(KD):
                    for k in range(MH):
                        nc.tensor.matmul(cpr[:, m, :], V1w[:, k, m * 128:(m + 1) * 128],
                                         hsS[:, k, :], start=(k == 0), stop=False)
                    nc.tensor.matmul(cpr[:, m, :], eye, b1b[:, m, :],
                                     start=False, stop=True)
                if t + 1 < T_RUN:
                    emit_u_prefix(t + 1)
                    emit_gp_prefix(t + 1)
                # cpf stored b-major so per-b layernorm views are contiguous
                # (gpsimd firmware requires contiguous [128, F] rows)
                cpf = smr.tile([128, B, KD], F32, tag="cpf")
                nc.scalar.activation(cpf[:, :, :].rearrange("p b k -> p k b"),
                                     cpr, AF.Gelu)
                # l2norm via gpsimd rmsnorm (no act-table); 1/sqrt(D) folded
                # into cw1 host-side.
                corePool = smr.tile([128, B, KD], F32, tag="corePool")
                for b in range(B):
                    nc.gpsimd.layernorm(corePool[:, b, :], cpf[:, b, :],
                                        subtract_mean=False, eps=1e-10)
                coreS = smr.tile([128, KD, B], BF, tag="coreS")
                nc.vector.tensor_copy(coreS,
                                      corePool[:, :, :].rearrange("p b k -> p k b"))
                gm = pr6.tile([128, 8, B], F32, tag="p6", name="gm")[:, :KD, :]
                for m in range(KD):
                    for k in range(KD):
                        nc.tensor.matmul(gm[:, m, :], cw1[:, k, m * 128:(m + 1) * 128],
                                         coreS[:, k, :], start=(k == 0), stop=False)
                    nc.tensor.matmul(gm[:, m, :], eye, cb1b[:, m, :],
                                     start=False, stop=True)
                gmS = smr.tile([128, KD, B], BF, tag="gmS")
                nc.scalar.activation(gmS, gm, AF.Gelu)
                cfp = pr6.tile([128, 8, B], F32, tag="p6", name="cfp")[:, :KD, :]
                for m in range(KD):
                    for k in range(KD):
                        nc.tensor.matmul(cfp[:, m, :], cw2[:, k, m * 128:(m + 1) * 128],
                                         gmS[:, k, :], start=(k == 0), stop=False)
                    nc.tensor.matmul(cfp[:, m, :], eye, cb2b[:, m, :],
                                     start=False, stop=True)
                cfF = smr.tile([128, KD, B], F32, tag="cfF")
                nc.scalar.activation(cfF, cfp, AF.Copy)
                # gate cf-branch folded through cp_w2: reads g1 (gmS) directly,
                # in parallel with the cfp/cfF branch
                gp = gp_tiles.pop(t)
                for m in range(KD):
                    for k in range(KD):
                        nc.tensor.matmul(gp[:, m, :], gw2[:, k, m * 128:(m + 1) * 128],
                                         gmS[:, k, :], start=False,
                                         stop=(m == KD - 1 and k == KD - 1))
                # gate via tanh (shares the gelu act table):
                #   sigmoid(x) = 0.5*(1 + tanh(x/2))
                #   hp = gate*(cf+tf-ctx) + ctx = 0.5*(tanh+1)*(cf+tf-ctx) + ctx
                tG = smr.tile([128, KD, B], F32, tag="gateS")
                nc.scalar.activation(tG, gp, AF.Tanh, scale=0.5)
                a1 = smr.tile([128, KD, B], F32, tag="a1")
                nc.vector.tensor_add(a1, cfF, w_t)
                q1 = smr.tile([128, KD, B], F32, tag="q1")
                nc.vector.scalar_tensor_tensor(q1, tG, 1.0, a1, OP.add, OP.mult)
                hp = smr.tile([128, B, KD], F32, tag="hp")
                nc.vector.scalar_tensor_tensor(
                    hp[:, :, :].rearrange("p b k -> p k b"),
                    q1, 0.5, ctx_t, OP.mult, OP.add)
                # LN + affine via gpsimd layernorm (no act-table), then clip.
                lnout = smr.tile([128, B, KD], F32, tag="lnout")
                for b in range(B):
                    nc.gpsimd.layernorm(lnout[:, b, :], hp[:, b, :],
                                        gamma_ap=outgS[:, :], beta_ap=outbS[:, :],
                                        subtract_mean=True, eps=1e-5)
                nc.vector.tensor_scalar(Hc[t // 8][:, :, (t % 8) * B:(t % 8) * B + B],
                                        lnout[:, :, :].rearrange("p b k -> p k b"),
                                        5.0, -5.0, OP.min, OP.max)
                if KDEBUG and t == min(7, T_RUN - 1):
                    nc.sync.dma_start(out=dbg_h[:, :],
                                      in_=Hc[0][:, :, :].rearrange("p k c -> p (k c)"))
                if KDEBUG and t == int(_os.environ.get("KPROBE_T", 0)):
                    for nm, tile_ in [("hs", hsS), ("cpf", cpf), ("core", coreS),
                                      ("gm", gmS), ("cf", cfF), ("tg", tG),
                                      ("hp", hp), ("ln", lnout)]:
                        nc.sync.dma_start(
                            out=dbg_st[nm][:, :],
                            in_=tile_[:, :, :].rearrange("p a b -> p (a b)"))
                # interleave lm-head work for the previous (complete) block
                if t >= 8 and T_RUN == T:
                    lm_chunks(mtb - 1, LM_SPLIT[t % 8], LM_SPLIT[t % 8 + 1])

            # lm-head tail: last block (or all blocks on truncated debug runs)
            tail_blocks = range(NBLK - 1, NBLK) if T_RUN == T else range(NBLK)
            for mt in tail_blocks:
                lm_chunks(mt, 0, NCH)

    nc.finalize()
    return nc


_NC_CACHE = {}


def _get_nc():
    key = (_FLAGS["ln_triv"], _FLAGS["bv0"])
    if key not in _NC_CACHE:
        _NC_CACHE[key] = build_nc()
    return _NC_CACHE[key]


def _prep_inputs(inputs):
    f = lambda x: np.asarray(x, np.float32)
    tok = np.asarray(inputs["token_ids"]).astype(np.int64)
    emb, pos = f(inputs["emb"]), f(inputs["pos_emb"])
    x0 = emb[tok.reshape(-1)] + np.tile(pos[:T], (B, 1))
    com = {"x0T": x0.T.astype(bfnp)}
    aiw, aib = f(inputs["attn_in_w"]), f(inputs["attn_in_b"])
    aow, aob = f(inputs["attn_out_w"]), f(inputs["attn_out_b"])
    for l in range(NL):
        wqk = np.zeros((D, 2048), np.float32)
        bqk = np.zeros(2048, np.float32)
        wv = np.zeros((D, 1024), np.float32)
        bv = np.zeros(1024, np.float32)
        wao = np.zeros((1024, D), np.float32)
        for h in range(NH):
            wqk[:, h * 128:h * 128 + HD] = aiw[l, h * HD:(h + 1) * HD, :].T
            wqk[:, 1024 + h * 128:1024 + h * 128 + HD] = aiw[l, D + h * HD:D + (h + 1) * HD, :].T
            bqk[h * 128:h * 128 + HD] = aib[l, h * HD:(h + 1) * HD]
            bqk[1024 + h * 128:1024 + h * 128 + HD] = aib[l, D + h * HD:D + (h + 1) * HD]
            wv[:, h * 128:h * 128 + HD] = aiw[l, 2 * D + h * HD:2 * D + (h + 1) * HD, :].T
            bv[h * 128:h * 128 + HD] = aib[l, 2 * D + h * HD:2 * D + (h + 1) * HD]
            wao[h * 128:h * 128 + HD, :] = aow[l][:, h * HD:(h + 1) * HD].T
        com[f"wqk{l}"] = wqk.astype(bfnp)
        com[f"bqk{l}"] = bqk.reshape(16, 128).T.copy()
        com[f"wv{l}"] = wv.astype(bfnp)
        com[f"bv{l}"] = bv.reshape(1, 1024).astype(bfnp)
        com[f"wao{l}"] = wao.astype(bfnp)
        com[f"bao{l}"] = aob[l].reshape(KD, 128).T.copy()
        com[f"ff1T{l}"] = f(inputs["ff_w1"])[l].T.astype(bfnp).copy()
        com[f"bff1{l}"] = f(inputs["ff_b1"])[l].reshape(MF, 128).T.copy()
        com[f"ff2T{l}"] = f(inputs["ff_w2"])[l].T.astype(bfnp).copy()
        com[f"bff2{l}"] = f(inputs["ff_b2"])[l].reshape(KD, 128).T.copy()
        com[f"g1{l}"] = f(inputs["n1_g"])[l].reshape(KD, 128).T.copy()
        com[f"bb1{l}"] = f(inputs["n1_b"])[l].reshape(KD, 128).T.copy()
        com[f"g2{l}"] = f(inputs["n2_g"])[l].reshape(KD, 128).T.copy()
        com[f"bb2{l}"] = f(inputs["n2_b"])[l].reshape(KD, 128).T.copy()
    com["encg"] = f(inputs["enc_norm_g"]).reshape(KD, 128).T.copy()
    com["encb"] = f(inputs["enc_norm_b"]).reshape(KD, 128).T.copy()
    tk, tq = np.meshgrid(np.arange(64), np.arange(64), indexing="ij")
    com["maskT"] = ((tk > tq) * -30000.0).astype(bfnp)
    com["eyed"] = np.eye(128, dtype=bfnp)
    com["V0d"] = f(inputs["V0"]).astype(bfnp)
    com["V1d"] = f(inputs["V1"]).astype(bfnp)
    R, tw = f(inputs["R"]), f(inputs["temp_w"])
    # folded recurrent weights: u += (alpha*R@V0)^T h ; tf = (alpha*R@tw^T)^T h
    com["Whud"] = (ALPHA * R @ f(inputs["V0"])).astype(bfnp)
    com["Wtfd"] = (ALPHA * R @ tw.T).astype(bfnp)
    # rmsnorm(x) = sqrt(D) * l2norm(x): fold the 1/sqrt(D) into cp_w1
    com["cw1d"] = (f(inputs["cp_w1"]).T / np.sqrt(D)).astype(bfnp).copy()
    com["cw2d"] = f(inputs["cp_w2"]).T.astype(bfnp).copy()
    gw = f(inputs["gate_w"])
    com["gw1d"] = gw[:, :D].T.astype(bfnp).copy()
    # gate cf-branch folded through cp_w2: gs_cf = Wgg^T g1 (+ gwB@cp_b2 -> gb)
    gwB = gw[:, D:]
    com["gw2d"] = np.ascontiguousarray((gwB @ f(inputs["cp_w2"])).T).astype(bfnp)
    com["gbd"] = (f(inputs["gate_b"]) + gwB @ f(inputs["cp_b2"])).reshape(KD, 128).T.copy()
    com["b0d"] = f(inputs["b0"]).reshape(MH, 128).T.copy()
    com["b1d"] = f(inputs["b1"]).reshape(KD, 128).T.copy()
    com["cb1d"] = f(inputs["cp_b1"]).reshape(KD, 128).T.copy()
    com["cb2d"] = f(inputs["cp_b2"]).reshape(KD, 128).T.copy()
    com["outgd"] = np.ascontiguousarray(f(inputs["out_g"]).reshape(KD, 128).T)
    com["outbd"] = np.ascontiguousarray(f(inputs["out_b"]).reshape(KD, 128).T)
    lmp = np.zeros((VP, D), np.float32)
    lmp[:V] = f(inputs["lm_head"])
    lmT = lmp.T.astype(bfnp)
    shards = [np.ascontiguousarray(lmT[:, c * VS:(c + 1) * VS]) for c in range(8)]
    return com, shards


LAST_RESULT = {}


def kernel(**inputs):
    import os
    f = lambda x: np.asarray(x, np.float32)
    _FLAGS["ln_triv"] = bool(
        all(np.all(f(inputs[k]) == 1.0) for k in ("n1_g", "n2_g", "enc_norm_g"))
        and all(np.all(f(inputs[k]) == 0.0) for k in ("n1_b", "n2_b", "enc_norm_b")))
    _FLAGS["bv0"] = bool(np.all(f(inputs["attn_in_b"])[:, 2 * D:] == 0.0))
    nc = _get_nc()
    com, shards = _prep_inputs(inputs)
    in_maps = [{**com, "lmTd": shards[c]} for c in range(8)]
    kw = {}
    if os.environ.get("KTRACE"):
        kw = dict(trace=True, tmpdir=os.environ.get("KTRACE_DIR", "/root/problem/trace_out"))
    res = run_bass_kernel_spmd(nc, in_maps, core_ids=list(range(8)), **kw)
    LAST_RESULT["res"] = res
    parts = [res.results[c]["out"] for c in range(8)]          # each [1024, VS], t-major rows
    full = np.concatenate(parts, axis=1)[:, :V].astype(np.float32)
    return np.ascontiguousarray(full.reshape(T, B, V).transpose(1, 0, 2))



# revision 63
# speedup vs baseline: 1.5290x; 1.0257x over previous
"""Trainium2 Bass kernel for nn_AgnisV5: 2-layer GPT encoder + gated
hierarchical recurrence + lm_head, SPMD over 8 NeuronCores.

Strategy: encoder + recurrence replicated on all cores (no collectives);
lm_head vocab-sharded 8 ways. Forward-pass simplification: stop_gradient is
identity, so the "settled" hierarchy path equals the differentiable shadow
path and blend == core_sh.

Layouts: feature-major activations [128p, K, N] (feature f = k*128 + p).
Encoder token columns are b-major (n = b*64 + t); recurrence/H/lm_head use
t-major (n = t*16 + b) via strided views of ctx.
"""
import numpy as np
import ml_dtypes
from contextlib import ExitStack

import concourse.bass as bass
import concourse.mybir as mybir
import concourse.tile as tile
from concourse import bacc
from concourse.bass_utils import run_bass_kernel_spmd

AF = mybir.ActivationFunctionType
OP = mybir.AluOpType
BF = mybir.dt.float16
F32 = mybir.dt.float32

D, HID, FFN, NH, NL, T, V, B = 768, 3072, 2048, 8, 2, 64, 50257, 16
HD = 96
KD = D // 128          # 6
MH = HID // 128        # 24
MF = FFN // 128        # 16
NTOK = B * T           # 1024
MT = NTOK // 128       # 8
VP = 53248             # padded vocab = 8*6656
VS = VP // 8           # 6656 per core
NCH = VS // 512        # 13
ALPHA = 0.4
QK_SCALE = 1.0 / np.sqrt(96.0)

bfnp = np.float16


_FLAGS = {"ln_triv": False, "bv0": False, "b2z": False}


def bc(ap, reps, pos):
    """Insert a stride-0 (broadcast) free dim at position pos (1-based over free dims)."""
    newap = list(ap.ap)
    newap.insert(pos, [0, reps])
    return bass.AP(tensor=ap.tensor, offset=ap.offset, ap=newap)


def build_nc():
    LN_TRIV = _FLAGS["ln_triv"]
    BV0 = _FLAGS["bv0"]
    B2Z = _FLAGS["b2z"]
    nc = bacc.Bacc(None, target_bir_lowering=False)

    # ---- DRAM I/O ----
    x0T = nc.dram_tensor("x0T", [D, NTOK], BF, kind="ExternalInput")
    enc_in = {}
    for l in range(NL):
        enc_in[f"wqk{l}"] = nc.dram_tensor(f"wqk{l}", [D, 2048], BF, kind="ExternalInput")
        enc_in[f"wv{l}"] = nc.dram_tensor(f"wv{l}", [D, 1024], BF, kind="ExternalInput")
        enc_in[f"wao{l}"] = nc.dram_tensor(f"wao{l}", [1024, D], BF, kind="ExternalInput")
        enc_in[f"ff1T{l}"] = nc.dram_tensor(f"ff1T{l}", [D, FFN], BF, kind="ExternalInput")
        enc_in[f"ff2T{l}"] = nc.dram_tensor(f"ff2T{l}", [FFN, D], BF, kind="ExternalInput")
        enc_in[f"bqk{l}"] = nc.dram_tensor(f"bqk{l}", [128, 16], F32, kind="ExternalInput")
        enc_in[f"bv{l}"] = nc.dram_tensor(f"bv{l}", [1, 1024], BF, kind="ExternalInput")
        enc_in[f"bao{l}"] = nc.dram_tensor(f"bao{l}", [128, KD], F32, kind="ExternalInput")
        enc_in[f"bff1{l}"] = nc.dram_tensor(f"bff1{l}", [128, MF], F32, kind="ExternalInput")
        enc_in[f"bff2{l}"] = nc.dram_tensor(f"bff2{l}", [128, KD], F32, kind="ExternalInput")
        enc_in[f"g1{l}"] = nc.dram_tensor(f"g1{l}", [128, KD], F32, kind="ExternalInput")
        enc_in[f"bb1{l}"] = nc.dram_tensor(f"bb1{l}", [128, KD], F32, kind="ExternalInput")
        enc_in[f"g2{l}"] = nc.dram_tensor(f"g2{l}", [128, KD], F32, kind="ExternalInput")
        enc_in[f"bb2{l}"] = nc.dram_tensor(f"bb2{l}", [128, KD], F32, kind="ExternalInput")
    encg = nc.dram_tensor("encg", [128, KD], F32, kind="ExternalInput")
    encb = nc.dram_tensor("encb", [128, KD], F32, kind="ExternalInput")
    maskT = nc.dram_tensor("maskT", [64, 64], BF, kind="ExternalInput")
    V0d = nc.dram_tensor("V0d", [D, HID], BF, kind="ExternalInput")
    V1d = nc.dram_tensor("V1d", [HID, D], BF, kind="ExternalInput")
    Whud = nc.dram_tensor("Whud", [D, HID], BF, kind="ExternalInput")
    Wtfd = nc.dram_tensor("Wtfd", [D, D], BF, kind="ExternalInput")
    cw1d = nc.dram_tensor("cw1d", [D, D], BF, kind="ExternalInput")
    cw2d = nc.dram_tensor("cw2d", [D, D], BF, kind="ExternalInput")
    gw1d = nc.dram_tensor("gw1d", [D, D], BF, kind="ExternalInput")
    gw2d = nc.dram_tensor("gw2d", [D, D], BF, kind="ExternalInput")
    b0d = nc.dram_tensor("b0d", [128, MH], F32, kind="ExternalInput")
    b1d = nc.dram_tensor("b1d", [128, KD], F32, kind="ExternalInput")
    cb1d = nc.dram_tensor("cb1d", [128, KD], F32, kind="ExternalInput")
    cb2d = nc.dram_tensor("cb2d", [128, KD], F32, kind="ExternalInput")
    gbd = nc.dram_tensor("gbd", [128, KD], F32, kind="ExternalInput")
    outgd = nc.dram_tensor("outgd", [128, KD], F32, kind="ExternalInput")
    outbd = nc.dram_tensor("outbd", [128, KD], F32, kind="ExternalInput")
    lmTd = nc.dram_tensor("lmTd", [D, VS], BF, kind="ExternalInput")
    eyed = nc.dram_tensor("eyed", [128, 128], BF, kind="ExternalInput")
    outd = nc.dram_tensor("out", [NTOK, VS], BF, kind="ExternalOutput")
    import os as _os0
    KDEBUG = bool(_os0.environ.get("KDEBUG"))
    if KDEBUG:
        dbg_ctx = nc.dram_tensor("dbg_ctx", [128, KD * NTOK], BF, kind="ExternalOutput")
        dbg_uc = nc.dram_tensor("dbg_uc", [128, MH * 128], BF, kind="ExternalOutput")
        dbg_h = nc.dram_tensor("dbg_h", [128, KD * 128], BF, kind="ExternalOutput")
        dbg_st = {}
        for nm, width, dt_ in [("hs", MH * B, BF), ("cpf", KD * B, F32),
                               ("core", KD * B, BF), ("gm", KD * B, BF),
                               ("cf", KD * B, F32), ("tg", KD * B, F32),
                               ("hp", KD * B, F32), ("ln", KD * B, F32)]:
            dbg_st[nm] = nc.dram_tensor(f"dbg_{nm}", [128, width], dt_,
                                        kind="ExternalOutput")

    def r3(h, p=128):
        return h[:, :].rearrange("(k p) n -> p k n", p=p)

    with ExitStack() as ctx:
        tc = ctx.enter_context(tile.TileContext(nc))
        cross = ctx.enter_context(tc.tile_pool(name="cross", bufs=1))

        # constants (cross-scope)
        ones_col = cross.tile([128, 1], BF, tag="ones_col")
        nc.vector.memset(ones_col, 1.0)
        ones64 = cross.tile([64, 1], BF, tag="ones64")
        nc.vector.memset(ones64, 1.0)
        ones_row_f = cross.tile([1, 128], F32, tag="ones_row_f")
        nc.vector.memset(ones_row_f, 1.0)
        ones_r64f = cross.tile([1, 64], F32, tag="ones_r64f")
        nc.vector.memset(ones_r64f, 1.0)
        mask_sb = cross.tile([64, 64], BF, tag="mask")
        nc.sync.dma_start(out=mask_sb, in_=maskT[:, :])
        eye = cross.tile([128, 128], BF, tag="eye")
        nc.sync.dma_start(out=eye, in_=eyed[:, :])
        eps5 = cross.tile([128, 1], F32, tag="eps5")
        nc.vector.memset(eps5, 1e-5)
        eps24 = cross.tile([128, 1], F32, tag="eps24")
        nc.vector.memset(eps24, 1e-24)
        ctxF = cross.tile([128, KD, NTOK], BF, tag="ctxF")

        # ================= ENCODER SCOPE =================
        with tc.tile_pool(name="wenc", bufs=1) as wenc, \
             tc.tile_pool(name="bige", bufs=1) as bige, \
             tc.tile_pool(name="sme", bufs=1) as sme, \
             tc.tile_pool(name="pmm", bufs=6, space="PSUM") as pmm, \
             tc.tile_pool(name="pbc", bufs=1, space="PSUM") as pbc:

            def ln_fm(x_sb, g_ap, b_ap, out, l2_out=None, triv=False):
                """x_sb [128, KD, 1024] bf16 -> LN -> out; optional l2 -> l2_out.
                triv: gamma==1 and beta==0 (runtime-specialized)."""
                for j in range(2):
                    jj = slice(j * 512, (j + 1) * 512)
                    s12 = pbc.tile([128, 2, 512], F32, tag="pabc", bufs=1,
                                   name="s12")[:1]
                    for k in range(KD):
                        sq = bige.tile([128, 512], BF, tag="sqc", bufs=2)
                        nc.vector.tensor_mul(sq, x_sb[:, k, jj], x_sb[:, k, jj])
                        nc.tensor.matmul(s12[:, 0, :], ones_col, x_sb[:, k, jj],
                                         start=(k == 0), stop=(k == KD - 1))
                        nc.tensor.matmul(s12[:, 1, :], ones_col, sq,
                                         start=(k == 0), stop=(k == KD - 1))
                    m = sme.tile([1, 512], F32, tag="mtag", bufs=2)
                    nc.vector.tensor_scalar_mul(m, s12[:, 0, :], 1.0 / D)
                    msq = sme.tile([1, 512], F32, tag="scr512", bufs=2)
                    nc.vector.tensor_mul(msq, m, m)
                    var = sme.tile([1, 512], F32, tag="scr512", bufs=2)
                    nc.vector.scalar_tensor_tensor(var, s12[:, 1, :], 1.0 / D,
                                                   msq, OP.mult, OP.subtract)
                    sd = sme.tile([1, 512], F32, tag="scr512", bufs=2)
                    nc.scalar.activation(sd, var, AF.Sqrt, bias=eps5[:1, :])
                    ac = sme.tile([1, 2, 512], F32, tag="acf")
                    nc.vector.reciprocal(ac[:, 0, :], sd)
                    nc.vector.scalar_tensor_tensor(ac[:, 1, :], m, -1.0,
                                                   ac[:, 0, :], OP.mult, OP.mult)
                    pac = pbc.tile([128, 2, 512], F32, tag="pabc", bufs=1)
                    for a in range(2):
                        nc.tensor.matmul(pac[:, a, :], ones_row_f, ac[:, a, :],
                                         start=True, stop=True)
                    pab = bige.tile([128, 2, 512], BF, tag="pab16", bufs=2)
                    nc.scalar.activation(pab, pac, AF.Copy)
                    for k in range(KD):
                        if triv:
                            t1 = bige.tile([128, 512], BF, tag="t1", bufs=2)
                            nc.vector.tensor_mul(t1, x_sb[:, k, jj], pab[:, 0, :])
                            nc.vector.tensor_add(out[:, k, jj], t1, pab[:, 1, :])
                        else:
                            t1 = bige.tile([128, 512], F32, tag="t1f", bufs=2)
                            nc.vector.tensor_mul(t1, x_sb[:, k, jj], pab[:, 0, :])
                            nc.vector.tensor_add(t1, t1, pab[:, 1, :])
                            nc.vector.tensor_scalar(out[:, k, jj], t1,
                                                    g_ap[:, k:k + 1],
                                                    b_ap[:, k:k + 1],
                                                    OP.mult, OP.add)
                if l2_out is None:
                    return
                for j in range(2):
                    jj = slice(j * 512, (j + 1) * 512)
                    s1t = pbc.tile([128, 2, 512], F32, tag="pabc", bufs=1,
                                   name="s1t")[:1]
                    s1 = s1t[:, 0, :]
                    for k in range(KD):
                        sq = bige.tile([128, 512], BF, tag="sqc", bufs=2)
                        nc.vector.tensor_mul(sq, out[:, k, jj], out[:, k, jj])
                        nc.tensor.matmul(s1, ones_col, sq,
                                         start=(k == 0), stop=(k == KD - 1))
                    sd = sme.tile([1, 512], F32, tag="scr512", bufs=2)
                    nc.scalar.activation(sd, s1, AF.Sqrt, bias=eps24[:1, :])
                    rr = sme.tile([1, 512], F32, tag="scr512", bufs=2)
                    nc.vector.reciprocal(rr, sd)
                    pat = pbc.tile([128, 2, 512], F32, tag="pabc", bufs=1)
                    pa = pat[:, 0, :]
                    nc.tensor.matmul(pa, ones_row_f, rr, start=True, stop=True)
                    pa16 = bige.tile([128, 512], BF, tag="pa16", bufs=2)
                    nc.scalar.activation(pa16, pa, AF.Copy)
                    for k in range(KD):
                        nc.vector.tensor_mul(l2_out[:, k, jj], out[:, k, jj], pa16)

            X = bige.tile([128, KD, NTOK], BF, tag="X")
            nc.sync.dma_start(out=X, in_=r3(x0T))
            gA = sme.tile([128, NL, KD], F32, tag="gA")
            bA = sme.tile([128, NL, KD], F32, tag="bA")
            gB = sme.tile([128, NL, KD], F32, tag="gB")
            bB = sme.tile([128, NL, KD], F32, tag="bB")
            for l in range(NL):
                nc.sync.dma_start(out=gA[:, l, :], in_=enc_in[f"g1{l}"][:, :])
                nc.sync.dma_start(out=bA[:, l, :], in_=enc_in[f"bb1{l}"][:, :])
                nc.sync.dma_start(out=gB[:, l, :], in_=enc_in[f"g2{l}"][:, :])
                nc.sync.dma_start(out=bB[:, l, :], in_=enc_in[f"bb2{l}"][:, :])

            for l in range(NL):
                XN = bige.tile([128, KD, NTOK], BF, tag="XN")
                ln_fm(X, gA[:, l, :], bA[:, l, :], XN, triv=LN_TRIV)
                wqk = wenc.tile([128, KD, 2048], BF, tag="wbig")
                nc.sync.dma_start(out=wqk, in_=r3(enc_in[f"wqk{l}"]))
                bqk = sme.tile([128, 16], F32, tag="bqk")
                nc.sync.dma_start(out=bqk, in_=enc_in[f"bqk{l}"][:, :])
                # v projection (token-major)
                wv = wenc.tile([128, KD, 1024], BF, tag="wsmall")
                nc.sync.dma_start(out=wv, in_=r3(enc_in[f"wv{l}"]))
                if not BV0:
                    ones_row = sme.tile([1, 128], BF, tag="ones_row")
                    nc.vector.memset(ones_row, 1.0)
                    bvr = sme.tile([1, 1024], BF, tag="bvr")
                    nc.sync.dma_start(out=bvr, in_=enc_in[f"bv{l}"][:, :])
                    bvs = bige.tile([128, 1024], BF, tag="bvs")
                    for j in range(2):
                        pb = pbc.tile([128, 2, 512], F32, tag="pabc", bufs=1)
                        nc.tensor.matmul(pb[:, 0, :], ones_row,
                                         bvr[:, j * 512:(j + 1) * 512],
                                         start=True, stop=True)
                        nc.vector.tensor_copy(bvs[:, j * 512:(j + 1) * 512],
                                              pb[:, 0, :])
                vS = bige.tile([64, B, 1024], BF, tag="vS")
                for b in range(B):
                    for j in range(2):
                        jj = slice(j * 512, (j + 1) * 512)
                        ps = pmm.tile([128, 512], F32, tag="pmm", name="psv")
                        for k in range(KD):
                            nc.tensor.matmul(ps[:64, :],
                                             XN[:, k, b * 64:(b + 1) * 64],
                                             wv[:, k, jj],
                                             start=(k == 0), stop=(k == KD - 1))
                        if not BV0:
                            nc.vector.tensor_add(vS[:, b, jj], ps[:64, :],
                                                 bvs[:64, jj])
                        elif (2 * b + j) % 2 == 0:
                            nc.scalar.activation(vS[:, b, jj], ps[:64, :], AF.Copy)
                        else:
                            nc.vector.tensor_copy(vS[:, b, jj], ps[:64, :])
                # attention per head; additive mask injected into score PSUM
                ofS = bige.tile([128, NH, NTOK], BF, tag="ofS")
                for h in range(NH):
                    qh = bige.tile([128, NTOK], BF, tag="qh", bufs=2)
                    kh = bige.tile([128, NTOK], BF, tag="kh", bufs=2)
                    for j in range(2):
                        jj = slice(j * 512, (j + 1) * 512)
                        ph = pmm.tile([128, 512], F32, tag="pmm", name="phq")
                        for k in range(KD):
                            nc.tensor.matmul(ph,
                                             wqk[:, k, h * 128:(h + 1) * 128],
                                             XN[:, k, jj],
                                             start=(k == 0), stop=(k == KD - 1))
                        if (h + j) % 2 == 0:
                            nc.scalar.activation(qh[:, jj], ph, AF.Identity,
                                                 bias=bqk[:, h:h + 1])
                        else:
                            nc.vector.tensor_scalar_add(qh[:, jj], ph,
                                                        bqk[:, h:h + 1])
                        ph2 = pmm.tile([128, 512], F32, tag="pmm", name="phk")
                        for k in range(KD):
                            nc.tensor.matmul(ph2,
                                             wqk[:, k, 1024 + h * 128:1024 + (h + 1) * 128],
                                             XN[:, k, jj],
                                             start=(k == 0), stop=(k == KD - 1))
                        if (h + j) % 2 == 0:
                            nc.vector.tensor_scalar_add(kh[:, jj], ph2,
                                                        bqk[:, 8 + h:9 + h])
                        else:
                            nc.scalar.activation(kh[:, jj], ph2, AF.Identity,
                                                 bias=bqk[:, 8 + h:9 + h])
                    eT = bige.tile([64, NTOK], BF, tag="eT", bufs=3)
                    for j in range(2):
                        jj = slice(j * 512, (j + 1) * 512)
                        sT = pmm.tile([128, 512], F32, tag="pmm", name="sT")
                        for b in range(8 * j, 8 * j + 8):
                            c = slice((b % 8) * 64, (b % 8) * 64 + 64)
                            nc.tensor.matmul(sT[:64, c],
                                             kh[:, b * 64:(b + 1) * 64],
                                             qh[:, b * 64:(b + 1) * 64],
                                             start=True, stop=False)
                            nc.tensor.matmul(sT[:64, c],
                                             eye[:64, :64], mask_sb,
                                             start=False, stop=True)
                        nc.scalar.activation(eT[:, jj], sT[:64, :], AF.Exp,
                                             scale=QK_SCALE)
                    ssum = pbc.tile([128, 2, 512], F32, tag="pabc", bufs=1,
                                    name="ssum")[:1]
                    for a in range(2):
                        nc.tensor.matmul(ssum[:, a, :], ones64,
                                         eT[:, a * 512:(a + 1) * 512],
                                         start=True, stop=True)
                    rs2 = sme.tile([1, 2, 512], F32, tag="acf")
                    rs = rs2[:, :, :].rearrange("p a c -> p (a c)")
                    nc.vector.reciprocal(rs, ssum[:, :, :].rearrange("p a c -> p (a c)"))
                    eN = bige.tile([64, NTOK], BF, tag="eN", bufs=3)
                    for a in range(2):
                        aa = slice(a * 512, (a + 1) * 512)
                        rb64 = pmm.tile([128, 512], F32, tag="pmm", name="rb64")
                        nc.tensor.matmul(rb64[:64, :], ones_r64f, rs[:, aa],
                                         start=True, stop=True)
                        nc.vector.tensor_mul(eN[:, aa], eT[:, aa], rb64[:64, :])
                    for j in range(2):
                        jj = slice(j * 512, (j + 1) * 512)
                        oh = pmm.tile([128, 512], F32, tag="pmm", name="oh")
                        for b in range(8 * j, 8 * j + 8):
                            c = slice((b % 8) * 64, (b % 8) * 64 + 64)
                            nc.tensor.matmul(oh[:, c],
                                             vS[:, b, h * 128:(h + 1) * 128],
                                             eN[:, b * 64:(b + 1) * 64],
                                             start=True, stop=True)
                        nc.scalar.activation(ofS[:, h, jj], oh, AF.Copy)
                # attn out + residual (in place on X)
                wao = wenc.tile([128, NH, D], BF, tag="wsmall")
                nc.sync.dma_start(out=wao,
                                  in_=enc_in[f"wao{l}"][:, :].rearrange("(k p) n -> p k n", p=128))
                bao = sme.tile([128, KD], F32, tag="bao")
                nc.sync.dma_start(out=bao, in_=enc_in[f"bao{l}"][:, :])
                for mo in range(KD):
                    for j in range(2):
                        jj = slice(j * 512, (j + 1) * 512)
                        ps = pmm.tile([128, 512], F32, tag="pmm", name="pswo")
                        for kv in range(NH):
                            nc.tensor.matmul(ps,
                                             wao[:, kv, mo * 128:(mo + 1) * 128],
                                             ofS[:, kv, jj],
                                             start=(kv == 0), stop=(kv == NH - 1))
                        if B2Z:
                            nc.vector.tensor_add(X[:, mo, jj], X[:, mo, jj], ps)
                        else:
                            t2 = bige.tile([128, 512], F32, tag="t2", bufs=2)
                            nc.vector.tensor_scalar_add(t2, ps, bao[:, mo:mo + 1])
                            nc.vector.tensor_add(X[:, mo, jj], X[:, mo, jj], t2)
                # ffn
                XN2 = bige.tile([128, KD, NTOK], BF, tag="XN")
                ln_fm(X, gB[:, l, :], bB[:, l, :], XN2, triv=LN_TRIV)
                w1 = wenc.tile([128, KD, FFN], BF, tag="wbig")
                nc.sync.dma_start(out=w1, in_=r3(enc_in[f"ff1T{l}"]))
                bf1 = sme.tile([128, MF], F32, tag="bf1")
                nc.sync.dma_start(out=bf1, in_=enc_in[f"bff1{l}"][:, :])
                G = bige.tile([128, MF, NTOK], BF, tag="G")
                for mf in range(MF):
                    for j in range(2):
                        jj = slice(j * 512, (j + 1) * 512)
                        ps = pmm.tile([128, 512], F32, tag="pmm", name="psf1")
                        for k in range(KD):
                            nc.tensor.matmul(ps,
                                             w1[:, k, mf * 128:(mf + 1) * 128],
                                             XN2[:, k, jj],
                                             start=(k == 0), stop=(k == KD - 1))
                        nc.scalar.activation(G[:, mf, jj], ps, AF.Gelu,
                                             bias=bf1[:, mf:mf + 1])
                w2 = wenc.tile([128, MF, D], BF, tag="wbig")
                nc.sync.dma_start(out=w2, in_=r3(enc_in[f"ff2T{l}"]))
                bf2 = sme.tile([128, KD], F32, tag="bao")
                nc.sync.dma_start(out=bf2, in_=enc_in[f"bff2{l}"][:, :])
                for mo in range(KD):
                    for j in range(2):
                        jj = slice(j * 512, (j + 1) * 512)
                        ps = pmm.tile([128, 512], F32, tag="pmm", name="psf2")
                        for kf in range(MF):
                            nc.tensor.matmul(ps,
                                             w2[:, kf, mo * 128:(mo + 1) * 128],
                                             G[:, kf, jj],
                                             start=(kf == 0), stop=(kf == MF - 1))
                        if B2Z:
                            nc.vector.tensor_add(X[:, mo, jj], X[:, mo, jj], ps)
                        else:
                            t2 = bige.tile([128, 512], F32, tag="t2", bufs=2)
                            nc.vector.tensor_scalar_add(t2, ps, bf2[:, mo:mo + 1])
                            nc.vector.tensor_add(X[:, mo, jj], X[:, mo, jj], t2)

            # final norm + l2 -> ctxF (cross pool)
            eg = sme.tile([128, KD], F32, tag="eg")
            eb = sme.tile([128, KD], F32, tag="eb")
            nc.sync.dma_start(out=eg, in_=encg[:, :])
            nc.sync.dma_start(out=eb, in_=encb[:, :])
            pre = bige.tile([128, KD, NTOK], BF, tag="XN")
            ln_fm(X, eg, eb, pre, l2_out=ctxF, triv=LN_TRIV)


        # ================= RECURRENCE SCOPE =================
        ctx_r = ctxF[:, :, :].rearrange("p k (b t) -> p t k b", b=B)
        with tc.tile_pool(name="wrec", bufs=1) as wrec, \
             tc.tile_pool(name="smr", bufs=2) as smr, \
             tc.tile_pool(name="prc", bufs=1, space="PSUM") as prc, \
             tc.tile_pool(name="pr6", bufs=1, space="PSUM") as pr6, \
             tc.tile_pool(name="prs", bufs=1, space="PSUM") as prs, \
             tc.tile_pool(name="pou", bufs=1, space="PSUM") as pou:

            Whu = wrec.tile([128, KD, HID], BF, tag="Whu")
            nc.sync.dma_start(out=Whu, in_=r3(Whud))
            V0w = wrec.tile([128, KD, HID], BF, tag="V0w")
            nc.sync.dma_start(out=V0w, in_=r3(V0d))
            gw1w = wrec.tile([128, KD, D], BF, tag="gw1w")
            nc.sync.dma_start(out=gw1w, in_=r3(gw1d))
            V1w = wrec.tile([128, MH, D], BF, tag="V1w")
            nc.sync.dma_start(out=V1w, in_=r3(V1d))
            Wtf = wrec.tile([128, KD, D], BF, tag="Wtf")
            nc.sync.dma_start(out=Wtf, in_=r3(Wtfd))
            cw1 = wrec.tile([128, KD, D], BF, tag="cw1")
            nc.sync.dma_start(out=cw1, in_=r3(cw1d))
            cw2 = wrec.tile([128, KD, D], BF, tag="cw2")
            nc.sync.dma_start(out=cw2, in_=r3(cw2d))
            gw2 = wrec.tile([128, KD, D], BF, tag="gw2")
            nc.sync.dma_start(out=gw2, in_=r3(gw2d))
            b1S = smr.tile([128, KD], F32, tag="b1S", bufs=1)
            nc.sync.dma_start(out=b1S, in_=b1d[:, :])
            cb1S = smr.tile([128, KD], F32, tag="cb1S", bufs=1)
            nc.sync.dma_start(out=cb1S, in_=cb1d[:, :])
            cb2S = smr.tile([128, KD], F32, tag="cb2S", bufs=1)
            nc.sync.dma_start(out=cb2S, in_=cb2d[:, :])
            # bf16 broadcast copies (B cols) for PSUM bias injection
            b0S = smr.tile([128, MH], F32, tag="b0S", bufs=1)
            nc.sync.dma_start(out=b0S, in_=b0d[:, :])
            b0b = smr.tile([128, MH, B], BF, tag="b0b", bufs=1)
            nc.vector.tensor_copy(b0b, bc(b0S[:, :], B, 2))
            gbS = smr.tile([128, KD], F32, tag="gbS", bufs=1)
            nc.sync.dma_start(out=gbS, in_=gbd[:, :])
            gbb = smr.tile([128, KD, B], BF, tag="gbb", bufs=1)
            nc.vector.tensor_copy(gbb, bc(gbS[:, :], B, 2))
            b1b = smr.tile([128, KD, B], BF, tag="b1b", bufs=1)
            nc.vector.tensor_copy(b1b, bc(b1S[:, :], B, 2))
            cb1b = smr.tile([128, KD, B], BF, tag="cb1b", bufs=1)
            nc.vector.tensor_copy(cb1b, bc(cb1S[:, :], B, 2))
            cb2b = smr.tile([128, KD, B], BF, tag="cb2b", bufs=1)
            nc.vector.tensor_copy(cb2b, bc(cb2S[:, :], B, 2))
            outgS = smr.tile([128, KD], F32, tag="outgS", bufs=1)
            nc.sync.dma_start(out=outgS, in_=outgd[:, :])
            outbS = smr.tile([128, KD], F32, tag="outbS", bufs=1)
            nc.sync.dma_start(out=outbS, in_=outbd[:, :])
            Hc = [wrec.tile([128, KD, 128], BF, tag=f"H{mt}", name=f"H{mt}")
                  for mt in range(MT)]

            import os as _os
            T_RUN = int(_os.environ.get("T_TRUNC", T))
            if T_RUN < T:
                for Hcm in Hc:
                    nc.vector.memset(Hcm, 0.0)

            NBLK = (T_RUN + 7) // 8
            if KDEBUG:
                nc.sync.dma_start(out=dbg_ctx[:, :],
                                  in_=ctxF[:, :, :].rearrange("p k n -> p (k n)"))

            # lm chunk emission: during step t of block w, process chunks of
            # block w-1 (Hc[w-1] is complete); 13 chunks spread over 8 steps.
            LM_SPLIT = [0, 2, 4, 6, 8, 10, 11, 12, 13]

            def lm_chunks(mt, j0, j1):
                for j in range(j0, j1):
                    lw = wrec.tile([128, KD, 512], BF, tag="lmw", bufs=2)
                    nc.sync.dma_start(
                        out=lw,
                        in_=lmTd[:, j * 512:(j + 1) * 512].rearrange("(k p) n -> p k n",
                                                                     p=128))
                    ps = pou.tile([128, 512], F32, tag="plm", bufs=2)
                    for k in range(KD):
                        nc.tensor.matmul(ps, Hc[mt][:, k, :], lw[:, k, :],
                                         start=(k == 0), stop=(k == KD - 1))
                    ot = smr.tile([128, 512], BF, tag="otile", bufs=3)
                    if j % 2 == 0:
                        nc.scalar.activation(ot, ps, AF.Copy)
                    else:
                        nc.vector.tensor_copy(ot, ps)
                    nc.sync.dma_start(out=outd[mt * 128:(mt + 1) * 128,
                                               j * 512:(j + 1) * 512], in_=ot)

            u_tiles = {}
            gp_tiles = {}

            def emit_u_prefix(tt, mlo=0, mhi=MH):
                # b0 + V0^T ctx accumulation prefix for step tt (h-independent),
                # emitted in chunks interleaved with Pool-LN calls so the PE
                # runs it during those idle windows. One psum bank: single
                # start (zeroes whole 2KB region), single stop at group end.
                if mlo == 0:
                    u_tiles[tt] = prc.tile([128, 32, B], F32, tag="pu", bufs=2,
                                           name=f"u{tt}")[:, :MH, :]
                un = u_tiles[tt]
                cx = ctx_r[:, tt]
                for m in range(mlo, mhi):
                    nc.tensor.matmul(un[:, m, :], eye, b0b[:, m, :],
                                     start=(m == 0), stop=False)
                    for k in range(KD):
                        nc.tensor.matmul(un[:, m, :], V0w[:, k, m * 128:(m + 1) * 128],
                                         cx[:, k, :], start=False,
                                         stop=(tt == 0 and m == MH - 1 and k == KD - 1))

            def emit_gp_prefix(tt, mlo=0, mhi=KD):
                if mlo == 0:
                    gp_tiles[tt] = pr6.tile([128, 8, B], F32, tag="pgp", bufs=2,
                                            name=f"gp{tt}")[:, :KD, :]
                gpn = gp_tiles[tt]
                cx = ctx_r[:, tt]
                for m in range(mlo, mhi):
                    nc.tensor.matmul(gpn[:, m, :], eye, gbb[:, m, :],
                                     start=(m == 0), stop=False)
                    for k in range(KD):
                        nc.tensor.matmul(gpn[:, m, :],
                                         gw1w[:, k, m * 128:(m + 1) * 128],
                                         cx[:, k, :], start=False, stop=False)

            emit_u_prefix(0)
            emit_gp_prefix(0)
            for t in range(T_RUN):
                ctx_t = ctx_r[:, t]
                mtb, off = t // 8, (t % 8) * B
                if t > 0:
                    hprev = Hc[(t - 1) // 8][:, :, ((t - 1) % 8) * B:((t - 1) % 8) * B + B]
                u = u_tiles.pop(t)
                if t > 0:
                    for m in range(MH):
                        for k in range(KD):
                            nc.tensor.matmul(u[:, m, :], Whu[:, k, m * 128:(m + 1) * 128],
                                             hprev[:, k, :], start=False,
                                             stop=(m == MH - 1 and k == KD - 1))
                hsS = smr.tile([128, MH, B], BF, tag="hsS")
                nc.scalar.activation(hsS, u, AF.Gelu)
                # tf branch (off critical path): w = tf - ctx
                w_t = smr.tile([128, KD, B], F32, tag="w_t")
                if t > 0:
                    tfp = prc.tile([128, 8, B], F32, tag="ptf", name="tfp")[:, :KD, :]
                    for m in range(KD):
                        for k in range(KD):
                            nc.tensor.matmul(tfp[:, m, :], Wtf[:, k, m * 128:(m + 1) * 128],
                                             hprev[:, k, :], start=(k == 0),
                                             stop=(k == KD - 1))
                    nc.vector.tensor_sub(w_t, tfp, ctx_t)
                else:
                    nc.vector.tensor_scalar_mul(w_t, ctx_t, -1.0)
                cpr = pr6.tile([128, 8, B], F32, tag="p6", name="cpr")[:, :KD, :]
                for m in range(KD):
                    for k in range(MH):
                        nc.tensor.matmul(cpr[:, m, :], V1w[:, k, m * 128:(m + 1) * 128],
                                         hsS[:, k, :], start=(k == 0), stop=False)
                    nc.tensor.matmul(cpr[:, m, :], eye, b1b[:, m, :],
                                     start=False, stop=True)
                # cpf stored b-major so per-b layernorm views are contiguous
                # (gpsimd firmware requires contiguous [128, F] rows)
                cpf = smr.tile([128, B, KD], F32, tag="cpf")
                nc.scalar.activation(cpf[:, :, :].rearrange("p b k -> p k b"),
                                     cpr, AF.Gelu)
                # l2norm via gpsimd rmsnorm (no act-table); 1/sqrt(D) folded
                # into cw1 host-side.
                corePool = smr.tile([128, B, KD], F32, tag="corePool")
                for b in range(B):
                    nc.gpsimd.layernorm(corePool[:, b, :], cpf[:, b, :],
                                        subtract_mean=False, eps=1e-10)
                    if b % 4 == 3 and t + 1 < T_RUN:
                        emit_u_prefix(t + 1, 6 * (b // 4), 6 * (b // 4 + 1))
                coreS = smr.tile([128, KD, B], BF, tag="coreS")
                nc.vector.tensor_copy(coreS,
                                      corePool[:, :, :].rearrange("p b k -> p k b"))
                gm = pr6.tile([128, 8, B], F32, tag="p6", name="gm")[:, :KD, :]
                for m in range(KD):
                    for k in range(KD):
                        nc.tensor.matmul(gm[:, m, :], cw1[:, k, m * 128:(m + 1) * 128],
                                         coreS[:, k, :], start=(k == 0), stop=False)
                    nc.tensor.matmul(gm[:, m, :], eye, cb1b[:, m, :],
                                     start=False, stop=True)
                gmS = smr.tile([128, KD, B], BF, tag="gmS")
                nc.scalar.activation(gmS, gm, AF.Gelu)
                cfp = pr6.tile([128, 8, B], F32, tag="p6", name="cfp")[:, :KD, :]
                for m in range(KD):
                    for k in range(KD):
                        nc.tensor.matmul(cfp[:, m, :], cw2[:, k, m * 128:(m + 1) * 128],
                                         gmS[:, k, :], start=(k == 0), stop=False)
                    nc.tensor.matmul(cfp[:, m, :], eye, cb2b[:, m, :],
                                     start=False, stop=True)
                cfF = smr.tile([128, KD, B], F32, tag="cfF")
                nc.scalar.activation(cfF, cfp, AF.Copy)
                # gate cf-branch folded through cp_w2: reads g1 (gmS) directly,
                # in parallel with the cfp/cfF branch
                gp = gp_tiles.pop(t)
                for m in range(KD):
                    for k in range(KD):
                        nc.tensor.matmul(gp[:, m, :], gw2[:, k, m * 128:(m + 1) * 128],
                                         gmS[:, k, :], start=False,
                                         stop=(m == KD - 1 and k == KD - 1))
                # gate via tanh (shares the gelu act table):
                #   sigmoid(x) = 0.5*(1 + tanh(x/2))
                #   hp = gate*(cf+tf-ctx) + ctx = 0.5*(tanh+1)*(cf+tf-ctx) + ctx
                tG = smr.tile([128, KD, B], F32, tag="gateS")
                nc.scalar.activation(tG, gp, AF.Tanh, scale=0.5)
                a1 = smr.tile([128, KD, B], F32, tag="a1")
                nc.vector.tensor_add(a1, cfF, w_t)
                q1 = smr.tile([128, KD, B], F32, tag="q1")
                nc.vector.scalar_tensor_tensor(q1, tG, 1.0, a1, OP.add, OP.mult)
                hp = smr.tile([128, B, KD], F32, tag="hp")
                nc.vector.scalar_tensor_tensor(
                    hp[:, :, :].rearrange("p b k -> p k b"),
                    q1, 0.5, ctx_t, OP.mult, OP.add)
                # LN + affine via gpsimd layernorm (no act-table), then clip.
                lnout = smr.tile([128, B, KD], F32, tag="lnout")
                for b in range(B):
                    nc.gpsimd.layernorm(lnout[:, b, :], hp[:, b, :],
                                        gamma_ap=outgS[:, :], beta_ap=outbS[:, :],
                                        subtract_mean=True, eps=1e-5)
                    if b % 8 == 7 and t + 1 < T_RUN:
                        emit_gp_prefix(t + 1, 3 * (b // 8), 3 * (b // 8 + 1))
                nc.vector.tensor_scalar(Hc[t // 8][:, :, (t % 8) * B:(t % 8) * B + B],
                                        lnout[:, :, :].rearrange("p b k -> p k b"),
                                        5.0, -5.0, OP.min, OP.max)
                if KDEBUG and t == min(7, T_RUN - 1):
                    nc.sync.dma_start(out=dbg_h[:, :],
                                      in_=Hc[0][:, :, :].rearrange("p k c -> p (k c)"))
                if KDEBUG and t == int(_os.environ.get("KPROBE_T", 0)):
                    for nm, tile_ in [("hs", hsS), ("cpf", cpf), ("core", coreS),
                                      ("gm", gmS), ("cf", cfF), ("tg", tG),
                                      ("hp", hp), ("ln", lnout)]:
                        nc.sync.dma_start(
                            out=dbg_st[nm][:, :],
                            in_=tile_[:, :, :].rearrange("p a b -> p (a b)"))
                # interleave lm-head work for the previous (complete) block
                if t >= 8 and T_RUN == T:
                    lm_chunks(mtb - 1, LM_SPLIT[t % 8], LM_SPLIT[t % 8 + 1])

            # lm-head tail: last block (or all blocks on truncated debug runs)
            tail_blocks = range(NBLK - 1, NBLK) if T_RUN == T else range(NBLK)
            for mt in tail_blocks:
                lm_chunks(mt, 0, NCH)

    nc.finalize()
    return nc


_NC_CACHE = {}


def _get_nc():
    key = (_FLAGS["ln_triv"], _FLAGS["bv0"], _FLAGS["b2z"])
    if key not in _NC_CACHE:
        _NC_CACHE[key] = build_nc()
    return _NC_CACHE[key]


def _prep_inputs(inputs):
    f = lambda x: np.asarray(x, np.float32)
    tok = np.asarray(inputs["token_ids"]).astype(np.int64)
    emb, pos = f(inputs["emb"]), f(inputs["pos_emb"])
    x0 = emb[tok.reshape(-1)] + np.tile(pos[:T], (B, 1))
    com = {"x0T": x0.T.astype(bfnp)}
    aiw, aib = f(inputs["attn_in_w"]), f(inputs["attn_in_b"])
    aow, aob = f(inputs["attn_out_w"]), f(inputs["attn_out_b"])
    for l in range(NL):
        wqk = np.zeros((D, 2048), np.float32)
        bqk = np.zeros(2048, np.float32)
        wv = np.zeros((D, 1024), np.float32)
        bv = np.zeros(1024, np.float32)
        wao = np.zeros((1024, D), np.float32)
        for h in range(NH):
            wqk[:, h * 128:h * 128 + HD] = aiw[l, h * HD:(h + 1) * HD, :].T
            wqk[:, 1024 + h * 128:1024 + h * 128 + HD] = aiw[l, D + h * HD:D + (h + 1) * HD, :].T
            bqk[h * 128:h * 128 + HD] = aib[l, h * HD:(h + 1) * HD]
            bqk[1024 + h * 128:1024 + h * 128 + HD] = aib[l, D + h * HD:D + (h + 1) * HD]
            wv[:, h * 128:h * 128 + HD] = aiw[l, 2 * D + h * HD:2 * D + (h + 1) * HD, :].T
            bv[h * 128:h * 128 + HD] = aib[l, 2 * D + h * HD:2 * D + (h + 1) * HD]
            wao[h * 128:h * 128 + HD, :] = aow[l][:, h * HD:(h + 1) * HD].T
        com[f"wqk{l}"] = wqk.astype(bfnp)
        com[f"bqk{l}"] = bqk.reshape(16, 128).T.copy()
        com[f"wv{l}"] = wv.astype(bfnp)
        com[f"bv{l}"] = bv.reshape(1, 1024).astype(bfnp)
        com[f"wao{l}"] = wao.astype(bfnp)
        com[f"bao{l}"] = aob[l].reshape(KD, 128).T.copy()
        com[f"ff1T{l}"] = f(inputs["ff_w1"])[l].T.astype(bfnp).copy()
        com[f"bff1{l}"] = f(inputs["ff_b1"])[l].reshape(MF, 128).T.copy()
        com[f"ff2T{l}"] = f(inputs["ff_w2"])[l].T.astype(bfnp).copy()
        com[f"bff2{l}"] = f(inputs["ff_b2"])[l].reshape(KD, 128).T.copy()
        com[f"g1{l}"] = f(inputs["n1_g"])[l].reshape(KD, 128).T.copy()
        com[f"bb1{l}"] = f(inputs["n1_b"])[l].reshape(KD, 128).T.copy()
        com[f"g2{l}"] = f(inputs["n2_g"])[l].reshape(KD, 128).T.copy()
        com[f"bb2{l}"] = f(inputs["n2_b"])[l].reshape(KD, 128).T.copy()
    com["encg"] = f(inputs["enc_norm_g"]).reshape(KD, 128).T.copy()
    com["encb"] = f(inputs["enc_norm_b"]).reshape(KD, 128).T.copy()
    tk, tq = np.meshgrid(np.arange(64), np.arange(64), indexing="ij")
    com["maskT"] = ((tk > tq) * -30000.0).astype(bfnp)
    com["eyed"] = np.eye(128, dtype=bfnp)
    com["V0d"] = f(inputs["V0"]).astype(bfnp)
    com["V1d"] = f(inputs["V1"]).astype(bfnp)
    R, tw = f(inputs["R"]), f(inputs["temp_w"])
    # folded recurrent weights: u += (alpha*R@V0)^T h ; tf = (alpha*R@tw^T)^T h
    com["Whud"] = (ALPHA * R @ f(inputs["V0"])).astype(bfnp)
    com["Wtfd"] = (ALPHA * R @ tw.T).astype(bfnp)
    # rmsnorm(x) = sqrt(D) * l2norm(x): fold the 1/sqrt(D) into cp_w1
    com["cw1d"] = (f(inputs["cp_w1"]).T / np.sqrt(D)).astype(bfnp).copy()
    com["cw2d"] = f(inputs["cp_w2"]).T.astype(bfnp).copy()
    gw = f(inputs["gate_w"])
    com["gw1d"] = gw[:, :D].T.astype(bfnp).copy()
    # gate cf-branch folded through cp_w2: gs_cf = Wgg^T g1 (+ gwB@cp_b2 -> gb)
    gwB = gw[:, D:]
    com["gw2d"] = np.ascontiguousarray((gwB @ f(inputs["cp_w2"])).T).astype(bfnp)
    com["gbd"] = (f(inputs["gate_b"]) + gwB @ f(inputs["cp_b2"])).reshape(KD, 128).T.copy()
    com["b0d"] = f(inputs["b0"]).reshape(MH, 128).T.copy()
    com["b1d"] = f(inputs["b1"]).reshape(KD, 128).T.copy()
    com["cb1d"] = f(inputs["cp_b1"]).reshape(KD, 128).T.copy()
    com["cb2d"] = f(inputs["cp_b2"]).reshape(KD, 128).T.copy()
    com["outgd"] = np.ascontiguousarray(f(inputs["out_g"]).reshape(KD, 128).T)
    com["outbd"] = np.ascontiguousarray(f(inputs["out_b"]).reshape(KD, 128).T)
    lmp = np.zeros((VP, D), np.float32)
    lmp[:V] = f(inputs["lm_head"])
    lmT = lmp.T.astype(bfnp)
    shards = [np.ascontiguousarray(lmT[:, c * VS:(c + 1) * VS]) for c in range(8)]
    return com, shards


LAST_RESULT = {}


def kernel(**inputs):
    import os
    f = lambda x: np.asarray(x, np.float32)
    _FLAGS["ln_triv"] = bool(
        all(np.all(f(inputs[k]) == 1.0) for k in ("n1_g", "n2_g", "enc_norm_g"))
        and all(np.all(f(inputs[k]) == 0.0) for k in ("n1_b", "n2_b", "enc_norm_b")))
    _FLAGS["bv0"] = bool(np.all(f(inputs["attn_in_b"])[:, 2 * D:] == 0.0))
    _FLAGS["b2z"] = bool(np.all(f(inputs["attn_out_b"]) == 0.0)
                         and np.all(f(inputs["ff_b2"]) == 0.0))
    nc = _get_nc()
    com, shards = _prep_inputs(inputs)
    in_maps = [{**com, "lmTd": shards[c]} for c in range(8)]
    kw = {}
    if os.environ.get("KTRACE"):
        kw = dict(trace=True, tmpdir=os.environ.get("KTRACE_DIR", "/root/problem/trace_out"))
    res = run_bass_kernel_spmd(nc, in_maps, core_ids=list(range(8)), **kw)
    LAST_RESULT["res"] = res
    parts = [res.results[c]["out"] for c in range(8)]          # each [1024, VS], t-major rows
    full = np.concatenate(parts, axis=1)[:, :V].astype(np.float32)
    return np.ascontiguousarray(full.reshape(T, B, V).transpose(1, 0, 2))



# revision 65
# speedup vs baseline: 1.5434x; 1.0094x over previous
"""Trainium2 Bass kernel for nn_AgnisV5: 2-layer GPT encoder + gated
hierarchical recurrence + lm_head, SPMD over 8 NeuronCores.

Strategy: encoder + recurrence replicated on all cores (no collectives);
lm_head vocab-sharded 8 ways. Forward-pass simplification: stop_gradient is
identity, so the "settled" hierarchy path equals the differentiable shadow
path and blend == core_sh.

Layouts: feature-major activations [128p, K, N] (feature f = k*128 + p).
Encoder token columns are b-major (n = b*64 + t); recurrence/H/lm_head use
t-major (n = t*16 + b) via strided views of ctx.
"""
import numpy as np
import ml_dtypes
from contextlib import ExitStack

import concourse.bass as bass
import concourse.mybir as mybir
import concourse.tile as tile
from concourse import bacc
from concourse.bass_utils import run_bass_kernel_spmd

AF = mybir.ActivationFunctionType
OP = mybir.AluOpType
BF = mybir.dt.float16
F32 = mybir.dt.float32

D, HID, FFN, NH, NL, T, V, B = 768, 3072, 2048, 8, 2, 64, 50257, 16
HD = 96
KD = D // 128          # 6
MH = HID // 128        # 24
MF = FFN // 128        # 16
NTOK = B * T           # 1024
MT = NTOK // 128       # 8
VP = 53248             # padded vocab = 8*6656
VS = VP // 8           # 6656 per core
NCH = VS // 512        # 13
ALPHA = 0.4
QK_SCALE = 1.0 / np.sqrt(96.0)

bfnp = np.float16


_FLAGS = {"ln_triv": False, "bv0": False, "b2z": False}


def bc(ap, reps, pos):
    """Insert a stride-0 (broadcast) free dim at position pos (1-based over free dims)."""
    newap = list(ap.ap)
    newap.insert(pos, [0, reps])
    return bass.AP(tensor=ap.tensor, offset=ap.offset, ap=newap)


def build_nc():
    LN_TRIV = _FLAGS["ln_triv"]
    BV0 = _FLAGS["bv0"]
    B2Z = _FLAGS["b2z"]
    nc = bacc.Bacc(None, target_bir_lowering=False)

    # ---- DRAM I/O ----
    x0T = nc.dram_tensor("x0T", [D, NTOK], BF, kind="ExternalInput")
    enc_in = {}
    for l in range(NL):
        enc_in[f"wqk{l}"] = nc.dram_tensor(f"wqk{l}", [D, 2048], BF, kind="ExternalInput")
        enc_in[f"wv{l}"] = nc.dram_tensor(f"wv{l}", [D, D], BF, kind="ExternalInput")
        enc_in[f"wao{l}"] = nc.dram_tensor(f"wao{l}", [1024, D], BF, kind="ExternalInput")
        enc_in[f"ff1T{l}"] = nc.dram_tensor(f"ff1T{l}", [D, FFN], BF, kind="ExternalInput")
        enc_in[f"ff2T{l}"] = nc.dram_tensor(f"ff2T{l}", [FFN, D], BF, kind="ExternalInput")
        enc_in[f"bqk{l}"] = nc.dram_tensor(f"bqk{l}", [128, 16], F32, kind="ExternalInput")
        enc_in[f"bv{l}"] = nc.dram_tensor(f"bv{l}", [1, D], BF, kind="ExternalInput")
        enc_in[f"bao{l}"] = nc.dram_tensor(f"bao{l}", [128, KD], F32, kind="ExternalInput")
        enc_in[f"bff1{l}"] = nc.dram_tensor(f"bff1{l}", [128, MF], F32, kind="ExternalInput")
        enc_in[f"bff2{l}"] = nc.dram_tensor(f"bff2{l}", [128, KD], F32, kind="ExternalInput")
        enc_in[f"g1{l}"] = nc.dram_tensor(f"g1{l}", [128, KD], F32, kind="ExternalInput")
        enc_in[f"bb1{l}"] = nc.dram_tensor(f"bb1{l}", [128, KD], F32, kind="ExternalInput")
        enc_in[f"g2{l}"] = nc.dram_tensor(f"g2{l}", [128, KD], F32, kind="ExternalInput")
        enc_in[f"bb2{l}"] = nc.dram_tensor(f"bb2{l}", [128, KD], F32, kind="ExternalInput")
    encg = nc.dram_tensor("encg", [128, KD], F32, kind="ExternalInput")
    encb = nc.dram_tensor("encb", [128, KD], F32, kind="ExternalInput")
    maskT = nc.dram_tensor("maskT", [64, 64], BF, kind="ExternalInput")
    V0d = nc.dram_tensor("V0d", [D, HID], BF, kind="ExternalInput")
    V1d = nc.dram_tensor("V1d", [HID, D], BF, kind="ExternalInput")
    Whud = nc.dram_tensor("Whud", [D, HID], BF, kind="ExternalInput")
    Wtfd = nc.dram_tensor("Wtfd", [D, D], BF, kind="ExternalInput")
    cw1d = nc.dram_tensor("cw1d", [D, D], BF, kind="ExternalInput")
    cw2d = nc.dram_tensor("cw2d", [D, D], BF, kind="ExternalInput")
    gw1d = nc.dram_tensor("gw1d", [D, D], BF, kind="ExternalInput")
    gw2d = nc.dram_tensor("gw2d", [D, D], BF, kind="ExternalInput")
    b0d = nc.dram_tensor("b0d", [128, MH], F32, kind="ExternalInput")
    b1d = nc.dram_tensor("b1d", [128, KD], F32, kind="ExternalInput")
    cb1d = nc.dram_tensor("cb1d", [128, KD], F32, kind="ExternalInput")
    cb2d = nc.dram_tensor("cb2d", [128, KD], F32, kind="ExternalInput")
    gbd = nc.dram_tensor("gbd", [128, KD], F32, kind="ExternalInput")
    outgd = nc.dram_tensor("outgd", [128, KD], F32, kind="ExternalInput")
    outbd = nc.dram_tensor("outbd", [128, KD], F32, kind="ExternalInput")
    lmTd = nc.dram_tensor("lmTd", [D, VS], BF, kind="ExternalInput")
    eyed = nc.dram_tensor("eyed", [128, 128], BF, kind="ExternalInput")
    outd = nc.dram_tensor("out", [NTOK, VS], BF, kind="ExternalOutput")
    import os as _os0
    KDEBUG = bool(_os0.environ.get("KDEBUG"))
    if KDEBUG:
        dbg_ctx = nc.dram_tensor("dbg_ctx", [128, KD * NTOK], BF, kind="ExternalOutput")
        dbg_uc = nc.dram_tensor("dbg_uc", [128, MH * 128], BF, kind="ExternalOutput")
        dbg_h = nc.dram_tensor("dbg_h", [128, KD * 128], BF, kind="ExternalOutput")
        dbg_st = {}
        for nm, width, dt_ in [("hs", MH * B, BF), ("cpf", KD * B, F32),
                               ("core", KD * B, BF), ("gm", KD * B, BF),
                               ("cf", KD * B, F32), ("tg", KD * B, F32),
                               ("hp", KD * B, F32), ("ln", KD * B, F32)]:
            dbg_st[nm] = nc.dram_tensor(f"dbg_{nm}", [128, width], dt_,
                                        kind="ExternalOutput")

    def r3(h, p=128):
        return h[:, :].rearrange("(k p) n -> p k n", p=p)

    with ExitStack() as ctx:
        tc = ctx.enter_context(tile.TileContext(nc))
        cross = ctx.enter_context(tc.tile_pool(name="cross", bufs=1))

        # constants (cross-scope)
        ones_col = cross.tile([128, 1], BF, tag="ones_col")
        nc.vector.memset(ones_col, 1.0)
        ones64 = cross.tile([64, 1], BF, tag="ones64")
        nc.vector.memset(ones64, 1.0)
        ones_row_f = cross.tile([1, 128], F32, tag="ones_row_f")
        nc.vector.memset(ones_row_f, 1.0)
        ones_r64f = cross.tile([1, 64], F32, tag="ones_r64f")
        nc.vector.memset(ones_r64f, 1.0)
        mask_sb = cross.tile([64, 64], BF, tag="mask")
        nc.sync.dma_start(out=mask_sb, in_=maskT[:, :])
        eye = cross.tile([128, 128], BF, tag="eye")
        nc.sync.dma_start(out=eye, in_=eyed[:, :])
        eps5 = cross.tile([128, 1], F32, tag="eps5")
        nc.vector.memset(eps5, 1e-5)
        eps24 = cross.tile([128, 1], F32, tag="eps24")
        nc.vector.memset(eps24, 1e-24)
        ctxF = cross.tile([128, KD, NTOK], BF, tag="ctxF")

        # ================= ENCODER SCOPE =================
        with tc.tile_pool(name="wenc", bufs=1) as wenc, \
             tc.tile_pool(name="bige", bufs=1) as bige, \
             tc.tile_pool(name="sme", bufs=1) as sme, \
             tc.tile_pool(name="pmm", bufs=6, space="PSUM") as pmm, \
             tc.tile_pool(name="pbc", bufs=1, space="PSUM") as pbc:

            def ln_fm(x_sb, g_ap, b_ap, out, l2_out=None, triv=False):
                """x_sb [128, KD, 1024] bf16 -> LN -> out; optional l2 -> l2_out.
                triv: gamma==1 and beta==0 (runtime-specialized)."""
                for j in range(2):
                    jj = slice(j * 512, (j + 1) * 512)
                    s12 = pbc.tile([128, 2, 512], F32, tag="pabc", bufs=1,
                                   name="s12")[:1]
                    for k in range(KD):
                        sq = bige.tile([128, 512], BF, tag="sqc", bufs=2)
                        nc.vector.tensor_mul(sq, x_sb[:, k, jj], x_sb[:, k, jj])
                        nc.tensor.matmul(s12[:, 0, :], ones_col, x_sb[:, k, jj],
                                         start=(k == 0), stop=(k == KD - 1))
                        nc.tensor.matmul(s12[:, 1, :], ones_col, sq,
                                         start=(k == 0), stop=(k == KD - 1))
                    m = sme.tile([1, 512], F32, tag="mtag", bufs=2)
                    nc.vector.tensor_scalar_mul(m, s12[:, 0, :], 1.0 / D)
                    msq = sme.tile([1, 512], F32, tag="scr512", bufs=2)
                    nc.vector.tensor_mul(msq, m, m)
                    var = sme.tile([1, 512], F32, tag="scr512", bufs=2)
                    nc.vector.scalar_tensor_tensor(var, s12[:, 1, :], 1.0 / D,
                                                   msq, OP.mult, OP.subtract)
                    sd = sme.tile([1, 512], F32, tag="scr512", bufs=2)
                    nc.scalar.activation(sd, var, AF.Sqrt, bias=eps5[:1, :])
                    ac = sme.tile([1, 2, 512], F32, tag="acf")
                    nc.vector.reciprocal(ac[:, 0, :], sd)
                    nc.vector.scalar_tensor_tensor(ac[:, 1, :], m, -1.0,
                                                   ac[:, 0, :], OP.mult, OP.mult)
                    pac = pbc.tile([128, 2, 512], F32, tag="pabc", bufs=1)
                    for a in range(2):
                        nc.tensor.matmul(pac[:, a, :], ones_row_f, ac[:, a, :],
                                         start=True, stop=True)
                    pab = bige.tile([128, 2, 512], BF, tag="pab16", bufs=2)
                    nc.scalar.activation(pab, pac, AF.Copy)
                    for k in range(KD):
                        if triv:
                            t1 = bige.tile([128, 512], BF, tag="t1", bufs=2)
                            nc.vector.tensor_mul(t1, x_sb[:, k, jj], pab[:, 0, :])
                            nc.vector.tensor_add(out[:, k, jj], t1, pab[:, 1, :])
                        else:
                            t1 = bige.tile([128, 512], F32, tag="t1f", bufs=2)
                            nc.vector.tensor_mul(t1, x_sb[:, k, jj], pab[:, 0, :])
                            nc.vector.tensor_add(t1, t1, pab[:, 1, :])
                            nc.vector.tensor_scalar(out[:, k, jj], t1,
                                                    g_ap[:, k:k + 1],
                                                    b_ap[:, k:k + 1],
                                                    OP.mult, OP.add)
                if l2_out is None:
                    return
                for j in range(2):
                    jj = slice(j * 512, (j + 1) * 512)
                    s1t = pbc.tile([128, 2, 512], F32, tag="pabc", bufs=1,
                                   name="s1t")[:1]
                    s1 = s1t[:, 0, :]
                    for k in range(KD):
                        sq = bige.tile([128, 512], BF, tag="sqc", bufs=2)
                        nc.vector.tensor_mul(sq, out[:, k, jj], out[:, k, jj])
                        nc.tensor.matmul(s1, ones_col, sq,
                                         start=(k == 0), stop=(k == KD - 1))
                    sd = sme.tile([1, 512], F32, tag="scr512", bufs=2)
                    nc.scalar.activation(sd, s1, AF.Sqrt, bias=eps24[:1, :])
                    rr = sme.tile([1, 512], F32, tag="scr512", bufs=2)
                    nc.vector.reciprocal(rr, sd)
                    pat = pbc.tile([128, 2, 512], F32, tag="pabc", bufs=1)
                    pa = pat[:, 0, :]
                    nc.tensor.matmul(pa, ones_row_f, rr, start=True, stop=True)
                    pa16 = bige.tile([128, 512], BF, tag="pa16", bufs=2)
                    nc.scalar.activation(pa16, pa, AF.Copy)
                    for k in range(KD):
                        nc.vector.tensor_mul(l2_out[:, k, jj], out[:, k, jj], pa16)

            X = bige.tile([128, KD, NTOK], BF, tag="X")
            nc.sync.dma_start(out=X, in_=r3(x0T))
            gA = sme.tile([128, NL, KD], F32, tag="gA")
            bA = sme.tile([128, NL, KD], F32, tag="bA")
            gB = sme.tile([128, NL, KD], F32, tag="gB")
            bB = sme.tile([128, NL, KD], F32, tag="bB")
            for l in range(NL):
                nc.sync.dma_start(out=gA[:, l, :], in_=enc_in[f"g1{l}"][:, :])
                nc.sync.dma_start(out=bA[:, l, :], in_=enc_in[f"bb1{l}"][:, :])
                nc.sync.dma_start(out=gB[:, l, :], in_=enc_in[f"g2{l}"][:, :])
                nc.sync.dma_start(out=bB[:, l, :], in_=enc_in[f"bb2{l}"][:, :])

            for l in range(NL):
                XN = bige.tile([128, KD, NTOK], BF, tag="XN")
                ln_fm(X, gA[:, l, :], bA[:, l, :], XN, triv=LN_TRIV)
                wqk = wenc.tile([128, KD, 2048], BF, tag="wbig")
                nc.sync.dma_start(out=wqk, in_=r3(enc_in[f"wqk{l}"]))
                bqk = sme.tile([128, 16], F32, tag="bqk")
                nc.sync.dma_start(out=bqk, in_=enc_in[f"bqk{l}"][:, :])
                # v projection (token-major)
                wv = wenc.tile([128, KD, D], BF, tag="wsmall")
                nc.sync.dma_start(out=wv, in_=r3(enc_in[f"wv{l}"]))
                if not BV0:
                    ones_row = sme.tile([1, 128], BF, tag="ones_row")
                    nc.vector.memset(ones_row, 1.0)
                    bvr = sme.tile([1, D], BF, tag="bvr")
                    nc.sync.dma_start(out=bvr, in_=enc_in[f"bv{l}"][:, :])
                    bvs = bige.tile([128, D], BF, tag="bvs")
                    for j in range(2):
                        pb = pbc.tile([128, 2, 512], F32, tag="pabc", bufs=1)
                        nc.tensor.matmul(pb[:, 0, :384], ones_row,
                                         bvr[:, j * 384:(j + 1) * 384],
                                         start=True, stop=True)
                        nc.vector.tensor_copy(bvs[:, j * 384:(j + 1) * 384],
                                              pb[:, 0, :384])
                vS = bige.tile([64, B, D], BF, tag="vS")
                for b in range(B):
                    for j in range(2):
                        jj = slice(j * 384, (j + 1) * 384)
                        ps = pmm.tile([128, 512], F32, tag="pmm", name="psv")
                        for k in range(KD):
                            nc.tensor.matmul(ps[:64, :384],
                                             XN[:, k, b * 64:(b + 1) * 64],
                                             wv[:, k, jj],
                                             start=(k == 0), stop=(k == KD - 1))
                        if not BV0:
                            nc.vector.tensor_add(vS[:, b, jj], ps[:64, :384],
                                                 bvs[:64, jj])
                        elif (2 * b + j) % 2 == 0:
                            nc.scalar.activation(vS[:, b, jj], ps[:64, :384],
                                                 AF.Copy)
                        else:
                            nc.vector.tensor_copy(vS[:, b, jj], ps[:64, :384])
                # attention per head; additive mask injected into score PSUM
                ofS = bige.tile([128, NH, NTOK], BF, tag="ofS")
                nc.vector.memset(ofS[96:128, :, :], 0.0)
                for h in range(NH):
                    qh = bige.tile([128, NTOK], BF, tag="qh", bufs=2)
                    kh = bige.tile([128, NTOK], BF, tag="kh", bufs=2)
                    for j in range(2):
                        jj = slice(j * 512, (j + 1) * 512)
                        ph = pmm.tile([128, 512], F32, tag="pmm", name="phq")
                        for k in range(KD):
                            nc.tensor.matmul(ph,
                                             wqk[:, k, h * 128:(h + 1) * 128],
                                             XN[:, k, jj],
                                             start=(k == 0), stop=(k == KD - 1))
                        if (h + j) % 2 == 0:
                            nc.scalar.activation(qh[:, jj], ph, AF.Identity,
                                                 bias=bqk[:, h:h + 1])
                        else:
                            nc.vector.tensor_scalar_add(qh[:, jj], ph,
                                                        bqk[:, h:h + 1])
                        ph2 = pmm.tile([128, 512], F32, tag="pmm", name="phk")
                        for k in range(KD):
                            nc.tensor.matmul(ph2,
                                             wqk[:, k, 1024 + h * 128:1024 + (h + 1) * 128],
                                             XN[:, k, jj],
                                             start=(k == 0), stop=(k == KD - 1))
                        if (h + j) % 2 == 0:
                            nc.vector.tensor_scalar_add(kh[:, jj], ph2,
                                                        bqk[:, 8 + h:9 + h])
                        else:
                            nc.scalar.activation(kh[:, jj], ph2, AF.Identity,
                                                 bias=bqk[:, 8 + h:9 + h])
                    eT = bige.tile([64, NTOK], BF, tag="eT", bufs=3)
                    for j in range(2):
                        jj = slice(j * 512, (j + 1) * 512)
                        sT = pmm.tile([128, 512], F32, tag="pmm", name="sT")
                        for b in range(8 * j, 8 * j + 8):
                            c = slice((b % 8) * 64, (b % 8) * 64 + 64)
                            nc.tensor.matmul(sT[:64, c],
                                             kh[:, b * 64:(b + 1) * 64],
                                             qh[:, b * 64:(b + 1) * 64],
                                             start=True, stop=False)
                            nc.tensor.matmul(sT[:64, c],
                                             eye[:64, :64], mask_sb,
                                             start=False, stop=True)
                        nc.scalar.activation(eT[:, jj], sT[:64, :], AF.Exp,
                                             scale=QK_SCALE)
                    ssum = pbc.tile([128, 2, 512], F32, tag="pabc", bufs=1,
                                    name="ssum")[:1]
                    for a in range(2):
                        nc.tensor.matmul(ssum[:, a, :], ones64,
                                         eT[:, a * 512:(a + 1) * 512],
                                         start=True, stop=True)
                    rs2 = sme.tile([1, 2, 512], F32, tag="acf")
                    rs = rs2[:, :, :].rearrange("p a c -> p (a c)")
                    nc.vector.reciprocal(rs, ssum[:, :, :].rearrange("p a c -> p (a c)"))
                    eN = bige.tile([64, NTOK], BF, tag="eN", bufs=3)
                    for a in range(2):
                        aa = slice(a * 512, (a + 1) * 512)
                        rb64 = pmm.tile([128, 512], F32, tag="pmm", name="rb64")
                        nc.tensor.matmul(rb64[:64, :], ones_r64f, rs[:, aa],
                                         start=True, stop=True)
                        nc.vector.tensor_mul(eN[:, aa], eT[:, aa], rb64[:64, :])
                    for j in range(2):
                        jj = slice(j * 512, (j + 1) * 512)
                        oh = pmm.tile([128, 512], F32, tag="pmm", name="oh")
                        for b in range(8 * j, 8 * j + 8):
                            c = slice((b % 8) * 64, (b % 8) * 64 + 64)
                            nc.tensor.matmul(oh[:HD, c],
                                             vS[:, b, h * HD:(h + 1) * HD],
                                             eN[:, b * 64:(b + 1) * 64],
                                             start=True, stop=True)
                        nc.scalar.activation(ofS[:HD, h, jj], oh[:HD, :], AF.Copy)
                # attn out + residual (in place on X)
                wao = wenc.tile([128, NH, D], BF, tag="wsmall")
                nc.sync.dma_start(out=wao,
                                  in_=enc_in[f"wao{l}"][:, :].rearrange("(k p) n -> p k n", p=128))
                bao = sme.tile([128, KD], F32, tag="bao")
                nc.sync.dma_start(out=bao, in_=enc_in[f"bao{l}"][:, :])
                for mo in range(KD):
                    for j in range(2):
                        jj = slice(j * 512, (j + 1) * 512)
                        ps = pmm.tile([128, 512], F32, tag="pmm", name="pswo")
                        for kv in range(NH):
                            nc.tensor.matmul(ps,
                                             wao[:, kv, mo * 128:(mo + 1) * 128],
                                             ofS[:, kv, jj],
                                             start=(kv == 0), stop=(kv == NH - 1))
                        if B2Z:
                            nc.vector.tensor_add(X[:, mo, jj], X[:, mo, jj], ps)
                        else:
                            t2 = bige.tile([128, 512], F32, tag="t2", bufs=2)
                            nc.vector.tensor_scalar_add(t2, ps, bao[:, mo:mo + 1])
                            nc.vector.tensor_add(X[:, mo, jj], X[:, mo, jj], t2)
                # ffn
                XN2 = bige.tile([128, KD, NTOK], BF, tag="XN")
                ln_fm(X, gB[:, l, :], bB[:, l, :], XN2, triv=LN_TRIV)
                w1 = wenc.tile([128, KD, FFN], BF, tag="wbig")
                nc.sync.dma_start(out=w1, in_=r3(enc_in[f"ff1T{l}"]))
                bf1 = sme.tile([128, MF], F32, tag="bf1")
                nc.sync.dma_start(out=bf1, in_=enc_in[f"bff1{l}"][:, :])
                G = bige.tile([128, MF, NTOK], BF, tag="G")
                for mf in range(MF):
                    for j in range(2):
                        jj = slice(j * 512, (j + 1) * 512)
                        ps = pmm.tile([128, 512], F32, tag="pmm", name="psf1")
                        for k in range(KD):
                            nc.tensor.matmul(ps,
                                             w1[:, k, mf * 128:(mf + 1) * 128],
                                             XN2[:, k, jj],
                                             start=(k == 0), stop=(k == KD - 1))
                        nc.scalar.activation(G[:, mf, jj], ps, AF.Gelu,
                                             bias=bf1[:, mf:mf + 1])
                w2 = wenc.tile([128, MF, D], BF, tag="wbig")
                nc.sync.dma_start(out=w2, in_=r3(enc_in[f"ff2T{l}"]))
                bf2 = sme.tile([128, KD], F32, tag="bao")
                nc.sync.dma_start(out=bf2, in_=enc_in[f"bff2{l}"][:, :])
                for mo in range(KD):
                    for j in range(2):
                        jj = slice(j * 512, (j + 1) * 512)
                        ps = pmm.tile([128, 512], F32, tag="pmm", name="psf2")
                        for kf in range(MF):
                            nc.tensor.matmul(ps,
                                             w2[:, kf, mo * 128:(mo + 1) * 128],
                                             G[:, kf, jj],
                                             start=(kf == 0), stop=(kf == MF - 1))
                        if B2Z:
                            nc.vector.tensor_add(X[:, mo, jj], X[:, mo, jj], ps)
                        else:
                            t2 = bige.tile([128, 512], F32, tag="t2", bufs=2)
                            nc.vector.tensor_scalar_add(t2, ps, bf2[:, mo:mo + 1])
                            nc.vector.tensor_add(X[:, mo, jj], X[:, mo, jj], t2)

            # final norm + l2 -> ctxF (cross pool)
            eg = sme.tile([128, KD], F32, tag="eg")
            eb = sme.tile([128, KD], F32, tag="eb")
            nc.sync.dma_start(out=eg, in_=encg[:, :])
            nc.sync.dma_start(out=eb, in_=encb[:, :])
            pre = bige.tile([128, KD, NTOK], BF, tag="XN")
            ln_fm(X, eg, eb, pre, l2_out=ctxF, triv=LN_TRIV)


        # ================= RECURRENCE SCOPE =================
        ctx_r = ctxF[:, :, :].rearrange("p k (b t) -> p t k b", b=B)
        with tc.tile_pool(name="wrec", bufs=1) as wrec, \
             tc.tile_pool(name="smr", bufs=2) as smr, \
             tc.tile_pool(name="prc", bufs=1, space="PSUM") as prc, \
             tc.tile_pool(name="pr6", bufs=1, space="PSUM") as pr6, \
             tc.tile_pool(name="prs", bufs=1, space="PSUM") as prs, \
             tc.tile_pool(name="pou", bufs=1, space="PSUM") as pou:

            Whu = wrec.tile([128, KD, HID], BF, tag="Whu")
            nc.sync.dma_start(out=Whu, in_=r3(Whud))
            V0w = wrec.tile([128, KD, HID], BF, tag="V0w")
            nc.sync.dma_start(out=V0w, in_=r3(V0d))
            gw1w = wrec.tile([128, KD, D], BF, tag="gw1w")
            nc.sync.dma_start(out=gw1w, in_=r3(gw1d))
            V1w = wrec.tile([128, MH, D], BF, tag="V1w")
            nc.sync.dma_start(out=V1w, in_=r3(V1d))
            Wtf = wrec.tile([128, KD, D], BF, tag="Wtf")
            nc.sync.dma_start(out=Wtf, in_=r3(Wtfd))
            cw1 = wrec.tile([128, KD, D], BF, tag="cw1")
            nc.sync.dma_start(out=cw1, in_=r3(cw1d))
            cw2 = wrec.tile([128, KD, D], BF, tag="cw2")
            nc.sync.dma_start(out=cw2, in_=r3(cw2d))
            gw2 = wrec.tile([128, KD, D], BF, tag="gw2")
            nc.sync.dma_start(out=gw2, in_=r3(gw2d))
            b1S = smr.tile([128, KD], F32, tag="b1S", bufs=1)
            nc.sync.dma_start(out=b1S, in_=b1d[:, :])
            cb1S = smr.tile([128, KD], F32, tag="cb1S", bufs=1)
            nc.sync.dma_start(out=cb1S, in_=cb1d[:, :])
            cb2S = smr.tile([128, KD], F32, tag="cb2S", bufs=1)
            nc.sync.dma_start(out=cb2S, in_=cb2d[:, :])
            # bf16 broadcast copies (B cols) for PSUM bias injection
            b0S = smr.tile([128, MH], F32, tag="b0S", bufs=1)
            nc.sync.dma_start(out=b0S, in_=b0d[:, :])
            b0b = smr.tile([128, MH, B], BF, tag="b0b", bufs=1)
            nc.vector.tensor_copy(b0b, bc(b0S[:, :], B, 2))
            gbS = smr.tile([128, KD], F32, tag="gbS", bufs=1)
            nc.sync.dma_start(out=gbS, in_=gbd[:, :])
            gbb = smr.tile([128, KD, B], BF, tag="gbb", bufs=1)
            nc.vector.tensor_copy(gbb, bc(gbS[:, :], B, 2))
            b1b = smr.tile([128, KD, B], BF, tag="b1b", bufs=1)
            nc.vector.tensor_copy(b1b, bc(b1S[:, :], B, 2))
            cb1b = smr.tile([128, KD, B], BF, tag="cb1b", bufs=1)
            nc.vector.tensor_copy(cb1b, bc(cb1S[:, :], B, 2))
            cb2b = smr.tile([128, KD, B], BF, tag="cb2b", bufs=1)
            nc.vector.tensor_copy(cb2b, bc(cb2S[:, :], B, 2))
            outgS = smr.tile([128, KD], F32, tag="outgS", bufs=1)
            nc.sync.dma_start(out=outgS, in_=outgd[:, :])
            outbS = smr.tile([128, KD], F32, tag="outbS", bufs=1)
            nc.sync.dma_start(out=outbS, in_=outbd[:, :])
            Hc = [wrec.tile([128, KD, 128], BF, tag=f"H{mt}", name=f"H{mt}")
                  for mt in range(MT)]

            import os as _os
            T_RUN = int(_os.environ.get("T_TRUNC", T))
            if T_RUN < T:
                for Hcm in Hc:
                    nc.vector.memset(Hcm, 0.0)

            NBLK = (T_RUN + 7) // 8
            if KDEBUG:
                nc.sync.dma_start(out=dbg_ctx[:, :],
                                  in_=ctxF[:, :, :].rearrange("p k n -> p (k n)"))

            # lm chunk emission: during step t of block w, process chunks of
            # block w-1 (Hc[w-1] is complete); 13 chunks spread over 8 steps.
            LM_SPLIT = [0, 2, 4, 6, 8, 10, 11, 12, 13]

            def lm_chunks(mt, j0, j1):
                for j in range(j0, j1):
                    lw = wrec.tile([128, KD, 512], BF, tag="lmw", bufs=2)
                    nc.sync.dma_start(
                        out=lw,
                        in_=lmTd[:, j * 512:(j + 1) * 512].rearrange("(k p) n -> p k n",
                                                                     p=128))
                    ps = pou.tile([128, 512], F32, tag="plm", bufs=2)
                    for k in range(KD):
                        nc.tensor.matmul(ps, Hc[mt][:, k, :], lw[:, k, :],
                                         start=(k == 0), stop=(k == KD - 1))
                    ot = smr.tile([128, 512], BF, tag="otile", bufs=3)
                    if j % 2 == 0:
                        nc.scalar.activation(ot, ps, AF.Copy)
                    else:
                        nc.vector.tensor_copy(ot, ps)
                    nc.sync.dma_start(out=outd[mt * 128:(mt + 1) * 128,
                                               j * 512:(j + 1) * 512], in_=ot)

            u_tiles = {}
            gp_tiles = {}

            def emit_u_prefix(tt, mlo=0, mhi=MH):
                # b0 + V0^T ctx accumulation prefix for step tt (h-independent),
                # emitted in chunks interleaved with Pool-LN calls so the PE
                # runs it during those idle windows. One psum bank: single
                # start (zeroes whole 2KB region), single stop at group end.
                if mlo == 0:
                    u_tiles[tt] = prc.tile([128, 32, B], F32, tag="pu", bufs=2,
                                           name=f"u{tt}")[:, :MH, :]
                un = u_tiles[tt]
                cx = ctx_r[:, tt]
                for m in range(mlo, mhi):
                    nc.tensor.matmul(un[:, m, :], eye, b0b[:, m, :],
                                     start=(m == 0), stop=False)
                    for k in range(KD):
                        nc.tensor.matmul(un[:, m, :], V0w[:, k, m * 128:(m + 1) * 128],
                                         cx[:, k, :], start=False,
                                         stop=(tt == 0 and m == MH - 1 and k == KD - 1))

            def emit_gp_prefix(tt, mlo=0, mhi=KD):
                if mlo == 0:
                    gp_tiles[tt] = pr6.tile([128, 8, B], F32, tag="pgp", bufs=2,
                                            name=f"gp{tt}")[:, :KD, :]
                gpn = gp_tiles[tt]
                cx = ctx_r[:, tt]
                for m in range(mlo, mhi):
                    nc.tensor.matmul(gpn[:, m, :], eye, gbb[:, m, :],
                                     start=(m == 0), stop=False)
                    for k in range(KD):
                        nc.tensor.matmul(gpn[:, m, :],
                                         gw1w[:, k, m * 128:(m + 1) * 128],
                                         cx[:, k, :], start=False, stop=False)

            emit_u_prefix(0)
            emit_gp_prefix(0)
            for t in range(T_RUN):
                ctx_t = ctx_r[:, t]
                mtb, off = t // 8, (t % 8) * B
                if t > 0:
                    hprev = Hc[(t - 1) // 8][:, :, ((t - 1) % 8) * B:((t - 1) % 8) * B + B]
                u = u_tiles.pop(t)
                if t > 0:
                    for m in range(MH):
                        for k in range(KD):
                            nc.tensor.matmul(u[:, m, :], Whu[:, k, m * 128:(m + 1) * 128],
                                             hprev[:, k, :], start=False,
                                             stop=(m == MH - 1 and k == KD - 1))
                hsS = smr.tile([128, MH, B], BF, tag="hsS")
                nc.scalar.activation(hsS, u, AF.Gelu)
                # tf branch (off critical path): w = tf - ctx
                w_t = smr.tile([128, KD, B], F32, tag="w_t")
                if t > 0:
                    tfp = prc.tile([128, 8, B], F32, tag="ptf", name="tfp")[:, :KD, :]
                    for m in range(KD):
                        for k in range(KD):
                            nc.tensor.matmul(tfp[:, m, :], Wtf[:, k, m * 128:(m + 1) * 128],
                                             hprev[:, k, :], start=(k == 0),
                                             stop=(k == KD - 1))
                    nc.vector.tensor_sub(w_t, tfp, ctx_t)
                else:
                    nc.vector.tensor_scalar_mul(w_t, ctx_t, -1.0)
                cpr = pr6.tile([128, 8, B], F32, tag="p6", name="cpr")[:, :KD, :]
                for m in range(KD):
                    for k in range(MH):
                        nc.tensor.matmul(cpr[:, m, :], V1w[:, k, m * 128:(m + 1) * 128],
                                         hsS[:, k, :], start=(k == 0), stop=False)
                    nc.tensor.matmul(cpr[:, m, :], eye, b1b[:, m, :],
                                     start=False, stop=True)
                # cpf stored b-major so per-b layernorm views are contiguous
                # (gpsimd firmware requires contiguous [128, F] rows)
                cpf = smr.tile([128, B, KD], F32, tag="cpf")
                nc.scalar.activation(cpf[:, :, :].rearrange("p b k -> p k b"),
                                     cpr, AF.Gelu)
                # l2norm via gpsimd rmsnorm (no act-table); 1/sqrt(D) folded
                # into cw1 host-side.
                corePool = smr.tile([128, B, KD], F32, tag="corePool")
                for b in range(B):
                    nc.gpsimd.layernorm(corePool[:, b, :], cpf[:, b, :],
                                        subtract_mean=False, eps=1e-10)
                    if b % 4 == 3 and t + 1 < T_RUN:
                        emit_u_prefix(t + 1, 6 * (b // 4), 6 * (b // 4 + 1))
                coreS = smr.tile([128, KD, B], BF, tag="coreS")
                nc.vector.tensor_copy(coreS,
                                      corePool[:, :, :].rearrange("p b k -> p k b"))
                gm = pr6.tile([128, 8, B], F32, tag="p6", name="gm")[:, :KD, :]
                for m in range(KD):
                    for k in range(KD):
                        nc.tensor.matmul(gm[:, m, :], cw1[:, k, m * 128:(m + 1) * 128],
                                         coreS[:, k, :], start=(k == 0), stop=False)
                    nc.tensor.matmul(gm[:, m, :], eye, cb1b[:, m, :],
                                     start=False, stop=True)
                gmS = smr.tile([128, KD, B], BF, tag="gmS")
                nc.scalar.activation(gmS, gm, AF.Gelu)
                cfp = pr6.tile([128, 8, B], F32, tag="p6", name="cfp")[:, :KD, :]
                for m in range(KD):
                    for k in range(KD):
                        nc.tensor.matmul(cfp[:, m, :], cw2[:, k, m * 128:(m + 1) * 128],
                                         gmS[:, k, :], start=(k == 0), stop=False)
                    nc.tensor.matmul(cfp[:, m, :], eye, cb2b[:, m, :],
                                     start=False, stop=True)
                cfF = smr.tile([128, KD, B], F32, tag="cfF")
                nc.scalar.activation(cfF, cfp, AF.Copy)
                # gate cf-branch folded through cp_w2: reads g1 (gmS) directly,
                # in parallel with the cfp/cfF branch
                gp = gp_tiles.pop(t)
                for m in range(KD):
                    for k in range(KD):
                        nc.tensor.matmul(gp[:, m, :], gw2[:, k, m * 128:(m + 1) * 128],
                                         gmS[:, k, :], start=False,
                                         stop=(m == KD - 1 and k == KD - 1))
                # gate via tanh (shares the gelu act table):
                #   sigmoid(x) = 0.5*(1 + tanh(x/2))
                #   hp = gate*(cf+tf-ctx) + ctx = 0.5*(tanh+1)*(cf+tf-ctx) + ctx
                tG = smr.tile([128, KD, B], F32, tag="gateS")
                nc.scalar.activation(tG, gp, AF.Tanh, scale=0.5)
                a1 = smr.tile([128, KD, B], F32, tag="a1")
                nc.vector.tensor_add(a1, cfF, w_t)
                q1 = smr.tile([128, KD, B], F32, tag="q1")
                nc.vector.scalar_tensor_tensor(q1, tG, 1.0, a1, OP.add, OP.mult)
                hp = smr.tile([128, B, KD], F32, tag="hp")
                nc.vector.scalar_tensor_tensor(
                    hp[:, :, :].rearrange("p b k -> p k b"),
                    q1, 0.5, ctx_t, OP.mult, OP.add)
                # LN + affine via gpsimd layernorm (no act-table), then clip.
                lnout = smr.tile([128, B, KD], F32, tag="lnout")
                for b in range(B):
                    nc.gpsimd.layernorm(lnout[:, b, :], hp[:, b, :],
                                        gamma_ap=outgS[:, :], beta_ap=outbS[:, :],
                                        subtract_mean=True, eps=1e-5)
                    if b % 8 == 7 and t + 1 < T_RUN:
                        emit_gp_prefix(t + 1, 3 * (b // 8), 3 * (b // 8 + 1))
                nc.vector.tensor_scalar(Hc[t // 8][:, :, (t % 8) * B:(t % 8) * B + B],
                                        lnout[:, :, :].rearrange("p b k -> p k b"),
                                        5.0, -5.0, OP.min, OP.max)
                if KDEBUG and t == min(7, T_RUN - 1):
                    nc.sync.dma_start(out=dbg_h[:, :],
                                      in_=Hc[0][:, :, :].rearrange("p k c -> p (k c)"))
                if KDEBUG and t == int(_os.environ.get("KPROBE_T", 0)):
                    for nm, tile_ in [("hs", hsS), ("cpf", cpf), ("core", coreS),
                                      ("gm", gmS), ("cf", cfF), ("tg", tG),
                                      ("hp", hp), ("ln", lnout)]:
                        nc.sync.dma_start(
                            out=dbg_st[nm][:, :],
                            in_=tile_[:, :, :].rearrange("p a b -> p (a b)"))
                # interleave lm-head work for the previous (complete) block
                if t >= 8 and T_RUN == T:
                    lm_chunks(mtb - 1, LM_SPLIT[t % 8], LM_SPLIT[t % 8 + 1])

            # lm-head tail: last block (or all blocks on truncated debug runs)
            tail_blocks = range(NBLK - 1, NBLK) if T_RUN == T else range(NBLK)
            for mt in tail_blocks:
                lm_chunks(mt, 0, NCH)

    nc.finalize()
    return nc


_NC_CACHE = {}


def _get_nc():
    key = (_FLAGS["ln_triv"], _FLAGS["bv0"], _FLAGS["b2z"])
    if key not in _NC_CACHE:
        _NC_CACHE[key] = build_nc()
    return _NC_CACHE[key]


def _prep_inputs(inputs):
    f = lambda x: np.asarray(x, np.float32)
    tok = np.asarray(inputs["token_ids"]).astype(np.int64)
    emb, pos = f(inputs["emb"]), f(inputs["pos_emb"])
    x0 = emb[tok.reshape(-1)] + np.tile(pos[:T], (B, 1))
    com = {"x0T": x0.T.astype(bfnp)}
    aiw, aib = f(inputs["attn_in_w"]), f(inputs["attn_in_b"])
    aow, aob = f(inputs["attn_out_w"]), f(inputs["attn_out_b"])
    for l in range(NL):
        wqk = np.zeros((D, 2048), np.float32)
        bqk = np.zeros(2048, np.float32)
        wv = np.zeros((D, D), np.float32)
        bv = np.zeros(D, np.float32)
        wao = np.zeros((1024, D), np.float32)
        for h in range(NH):
            wqk[:, h * 128:h * 128 + HD] = aiw[l, h * HD:(h + 1) * HD, :].T
            wqk[:, 1024 + h * 128:1024 + h * 128 + HD] = aiw[l, D + h * HD:D + (h + 1) * HD, :].T
            bqk[h * 128:h * 128 + HD] = aib[l, h * HD:(h + 1) * HD]
            bqk[1024 + h * 128:1024 + h * 128 + HD] = aib[l, D + h * HD:D + (h + 1) * HD]
            wv[:, h * HD:(h + 1) * HD] = aiw[l, 2 * D + h * HD:2 * D + (h + 1) * HD, :].T
            bv[h * HD:(h + 1) * HD] = aib[l, 2 * D + h * HD:2 * D + (h + 1) * HD]
            wao[h * 128:h * 128 + HD, :] = aow[l][:, h * HD:(h + 1) * HD].T
        com[f"wqk{l}"] = wqk.astype(bfnp)
        com[f"bqk{l}"] = bqk.reshape(16, 128).T.copy()
        com[f"wv{l}"] = wv.astype(bfnp)
        com[f"bv{l}"] = bv.reshape(1, D).astype(bfnp)
        com[f"wao{l}"] = wao.astype(bfnp)
        com[f"bao{l}"] = aob[l].reshape(KD, 128).T.copy()
        com[f"ff1T{l}"] = f(inputs["ff_w1"])[l].T.astype(bfnp).copy()
        com[f"bff1{l}"] = f(inputs["ff_b1"])[l].reshape(MF, 128).T.copy()
        com[f"ff2T{l}"] = f(inputs["ff_w2"])[l].T.astype(bfnp).copy()
        com[f"bff2{l}"] = f(inputs["ff_b2"])[l].reshape(KD, 128).T.copy()
        com[f"g1{l}"] = f(inputs["n1_g"])[l].reshape(KD, 128).T.copy()
        com[f"bb1{l}"] = f(inputs["n1_b"])[l].reshape(KD, 128).T.copy()
        com[f"g2{l}"] = f(inputs["n2_g"])[l].reshape(KD, 128).T.copy()
        com[f"bb2{l}"] = f(inputs["n2_b"])[l].reshape(KD, 128).T.copy()
    com["encg"] = f(inputs["enc_norm_g"]).reshape(KD, 128).T.copy()
    com["encb"] = f(inputs["enc_norm_b"]).reshape(KD, 128).T.copy()
    tk, tq = np.meshgrid(np.arange(64), np.arange(64), indexing="ij")
    com["maskT"] = ((tk > tq) * -30000.0).astype(bfnp)
    com["eyed"] = np.eye(128, dtype=bfnp)
    com["V0d"] = f(inputs["V0"]).astype(bfnp)
    com["V1d"] = f(inputs["V1"]).astype(bfnp)
    R, tw = f(inputs["R"]), f(inputs["temp_w"])
    # folded recurrent weights: u += (alpha*R@V0)^T h ; tf = (alpha*R@tw^T)^T h
    com["Whud"] = (ALPHA * R @ f(inputs["V0"])).astype(bfnp)
    com["Wtfd"] = (ALPHA * R @ tw.T).astype(bfnp)
    # rmsnorm(x) = sqrt(D) * l2norm(x): fold the 1/sqrt(D) into cp_w1
    com["cw1d"] = (f(inputs["cp_w1"]).T / np.sqrt(D)).astype(bfnp).copy()
    com["cw2d"] = f(inputs["cp_w2"]).T.astype(bfnp).copy()
    gw = f(inputs["gate_w"])
    com["gw1d"] = gw[:, :D].T.astype(bfnp).copy()
    # gate cf-branch folded through cp_w2: gs_cf = Wgg^T g1 (+ gwB@cp_b2 -> gb)
    gwB = gw[:, D:]
    com["gw2d"] = np.ascontiguousarray((gwB @ f(inputs["cp_w2"])).T).astype(bfnp)
    com["gbd"] = (f(inputs["gate_b"]) + gwB @ f(inputs["cp_b2"])).reshape(KD, 128).T.copy()
    com["b0d"] = f(inputs["b0"]).reshape(MH, 128).T.copy()
    com["b1d"] = f(inputs["b1"]).reshape(KD, 128).T.copy()
    com["cb1d"] = f(inputs["cp_b1"]).reshape(KD, 128).T.copy()
    com["cb2d"] = f(inputs["cp_b2"]).reshape(KD, 128).T.copy()
    com["outgd"] = np.ascontiguousarray(f(inputs["out_g"]).reshape(KD, 128).T)
    com["outbd"] = np.ascontiguousarray(f(inputs["out_b"]).reshape(KD, 128).T)
    lmp = np.zeros((VP, D), np.float32)
    lmp[:V] = f(inputs["lm_head"])
    lmT = lmp.T.astype(bfnp)
    shards = [np.ascontiguousarray(lmT[:, c * VS:(c + 1) * VS]) for c in range(8)]
    return com, shards


LAST_RESULT = {}


def kernel(**inputs):
    import os
    f = lambda x: np.asarray(x, np.float32)
    _FLAGS["ln_triv"] = bool(
        all(np.all(f(inputs[k]) == 1.0) for k in ("n1_g", "n2_g", "enc_norm_g"))
        and all(np.all(f(inputs[k]) == 0.0) for k in ("n1_b", "n2_b", "enc_norm_b")))
    _FLAGS["bv0"] = bool(np.all(f(inputs["attn_in_b"])[:, 2 * D:] == 0.0))
    _FLAGS["b2z"] = bool(np.all(f(inputs["attn_out_b"]) == 0.0)
                         and np.all(f(inputs["ff_b2"]) == 0.0))
    nc = _get_nc()
    com, shards = _prep_inputs(inputs)
    in_maps = [{**com, "lmTd": shards[c]} for c in range(8)]
    kw = {}
    if os.environ.get("KTRACE"):
        kw = dict(trace=True, tmpdir=os.environ.get("KTRACE_DIR", "/root/problem/trace_out"))
    res = run_bass_kernel_spmd(nc, in_maps, core_ids=list(range(8)), **kw)
    LAST_RESULT["res"] = res
    parts = [res.results[c]["out"] for c in range(8)]          # each [1024, VS], t-major rows
    full = np.concatenate(parts, axis=1)[:, :V].astype(np.float32)
    return np.ascontiguousarray(full.reshape(T, B, V).transpose(1, 0, 2))



# revision 66
# speedup vs baseline: 1.6151x; 1.0465x over previous
"""Trainium2 Bass kernel for nn_AgnisV5: 2-layer GPT encoder + gated
hierarchical recurrence + lm_head, SPMD over 8 NeuronCores.

Strategy: encoder + recurrence replicated on all cores (no collectives);
lm_head vocab-sharded 8 ways. Forward-pass simplification: stop_gradient is
identity, so the "settled" hierarchy path equals the differentiable shadow
path and blend == core_sh.

Layouts: feature-major activations [128p, K, N] (feature f = k*128 + p).
Encoder token columns are b-major (n = b*64 + t); recurrence/H/lm_head use
t-major (n = t*16 + b) via strided views of ctx.
"""
import numpy as np
import ml_dtypes
from contextlib import ExitStack

import concourse.bass as bass
import concourse.mybir as mybir
import concourse.tile as tile
from concourse import bacc
from concourse.bass_utils import run_bass_kernel_spmd

AF = mybir.ActivationFunctionType
OP = mybir.AluOpType
BF = mybir.dt.float16
F32 = mybir.dt.float32

D, HID, FFN, NH, NL, T, V, B = 768, 3072, 2048, 8, 2, 64, 50257, 16
HD = 96
KD = D // 128          # 6
MH = HID // 128        # 24
MF = FFN // 128        # 16
NTOK = B * T           # 1024
MT = NTOK // 128       # 8
VP = 53248             # padded vocab = 8*6656
VS = VP // 8           # 6656 per core
NCH = VS // 512        # 13
ALPHA = 0.4
QK_SCALE = 1.0 / np.sqrt(96.0)

bfnp = np.float16


_FLAGS = {"ln_triv": False, "bv0": False, "b2z": False}


def bc(ap, reps, pos):
    """Insert a stride-0 (broadcast) free dim at position pos (1-based over free dims)."""
    newap = list(ap.ap)
    newap.insert(pos, [0, reps])
    return bass.AP(tensor=ap.tensor, offset=ap.offset, ap=newap)


def build_nc():
    LN_TRIV = _FLAGS["ln_triv"]
    BV0 = _FLAGS["bv0"]
    B2Z = _FLAGS["b2z"]
    nc = bacc.Bacc(None, target_bir_lowering=False)

    # ---- DRAM I/O ----
    x0T = nc.dram_tensor("x0T", [D, NTOK], BF, kind="ExternalInput")
    enc_in = {}
    for l in range(NL):
        enc_in[f"wqk{l}"] = nc.dram_tensor(f"wqk{l}", [D, 2048], BF, kind="ExternalInput")
        enc_in[f"wv{l}"] = nc.dram_tensor(f"wv{l}", [D, D], BF, kind="ExternalInput")
        enc_in[f"wao{l}"] = nc.dram_tensor(f"wao{l}", [1024, D], BF, kind="ExternalInput")
        enc_in[f"ff1T{l}"] = nc.dram_tensor(f"ff1T{l}", [D, FFN], BF, kind="ExternalInput")
        enc_in[f"ff2T{l}"] = nc.dram_tensor(f"ff2T{l}", [FFN, D], BF, kind="ExternalInput")
        enc_in[f"bqk{l}"] = nc.dram_tensor(f"bqk{l}", [128, 16], F32, kind="ExternalInput")
        enc_in[f"bv{l}"] = nc.dram_tensor(f"bv{l}", [1, D], BF, kind="ExternalInput")
        enc_in[f"bao{l}"] = nc.dram_tensor(f"bao{l}", [128, KD], F32, kind="ExternalInput")
        enc_in[f"bff1{l}"] = nc.dram_tensor(f"bff1{l}", [128, MF], F32, kind="ExternalInput")
        enc_in[f"bff2{l}"] = nc.dram_tensor(f"bff2{l}", [128, KD], F32, kind="ExternalInput")
        enc_in[f"g1{l}"] = nc.dram_tensor(f"g1{l}", [128, KD], F32, kind="ExternalInput")
        enc_in[f"bb1{l}"] = nc.dram_tensor(f"bb1{l}", [128, KD], F32, kind="ExternalInput")
        enc_in[f"g2{l}"] = nc.dram_tensor(f"g2{l}", [128, KD], F32, kind="ExternalInput")
        enc_in[f"bb2{l}"] = nc.dram_tensor(f"bb2{l}", [128, KD], F32, kind="ExternalInput")
    encg = nc.dram_tensor("encg", [128, KD], F32, kind="ExternalInput")
    encb = nc.dram_tensor("encb", [128, KD], F32, kind="ExternalInput")
    maskT = nc.dram_tensor("maskT", [64, 64], BF, kind="ExternalInput")
    V0d = nc.dram_tensor("V0d", [D, HID], BF, kind="ExternalInput")
    V1d = nc.dram_tensor("V1d", [HID, D], BF, kind="ExternalInput")
    Whud = nc.dram_tensor("Whud", [D, HID], BF, kind="ExternalInput")
    Wtfd = nc.dram_tensor("Wtfd", [D, D], BF, kind="ExternalInput")
    cw1d = nc.dram_tensor("cw1d", [D, D], BF, kind="ExternalInput")
    cw2d = nc.dram_tensor("cw2d", [D, D], BF, kind="ExternalInput")
    gw1d = nc.dram_tensor("gw1d", [D, D], BF, kind="ExternalInput")
    gw2d = nc.dram_tensor("gw2d", [D, D], BF, kind="ExternalInput")
    b0d = nc.dram_tensor("b0d", [128, MH], F32, kind="ExternalInput")
    b1d = nc.dram_tensor("b1d", [128, KD], F32, kind="ExternalInput")
    cb1d = nc.dram_tensor("cb1d", [128, KD], F32, kind="ExternalInput")
    cb2d = nc.dram_tensor("cb2d", [128, KD], F32, kind="ExternalInput")
    gbd = nc.dram_tensor("gbd", [128, KD], F32, kind="ExternalInput")
    outgd = nc.dram_tensor("outgd", [128, KD], F32, kind="ExternalInput")
    outbd = nc.dram_tensor("outbd", [128, KD], F32, kind="ExternalInput")
    lmTd = nc.dram_tensor("lmTd", [D, VS], BF, kind="ExternalInput")
    eyed = nc.dram_tensor("eyed", [128, 128], BF, kind="ExternalInput")
    outd = nc.dram_tensor("out", [NTOK, VS], BF, kind="ExternalOutput")
    import os as _os0
    KDEBUG = bool(_os0.environ.get("KDEBUG"))
    if KDEBUG:
        dbg_ctx = nc.dram_tensor("dbg_ctx", [128, KD * NTOK], BF, kind="ExternalOutput")
        dbg_uc = nc.dram_tensor("dbg_uc", [128, MH * 128], BF, kind="ExternalOutput")
        dbg_h = nc.dram_tensor("dbg_h", [128, KD * 128], BF, kind="ExternalOutput")
        dbg_st = {}
        for nm, width, dt_ in [("hs", MH * B, BF), ("cpf", KD * B, F32),
                               ("core", KD * B, BF), ("gm", KD * B, BF),
                               ("cf", KD * B, F32), ("tg", KD * B, F32),
                               ("hp", KD * B, F32), ("ln", KD * B, F32)]:
            dbg_st[nm] = nc.dram_tensor(f"dbg_{nm}", [128, width], dt_,
                                        kind="ExternalOutput")

    def r3(h, p=128):
        return h[:, :].rearrange("(k p) n -> p k n", p=p)

    with ExitStack() as ctx:
        tc = ctx.enter_context(tile.TileContext(nc))
        cross = ctx.enter_context(tc.tile_pool(name="cross", bufs=1))

        # constants (cross-scope)
        ones_col = cross.tile([128, 1], BF, tag="ones_col")
        nc.vector.memset(ones_col, 1.0)
        ones64 = cross.tile([64, 1], BF, tag="ones64")
        nc.vector.memset(ones64, 1.0)
        ones_row_f = cross.tile([1, 128], F32, tag="ones_row_f")
        nc.vector.memset(ones_row_f, 1.0)
        ones_r64f = cross.tile([1, 64], F32, tag="ones_r64f")
        nc.vector.memset(ones_r64f, 1.0)
        mask_sb = cross.tile([64, 64], BF, tag="mask")
        nc.sync.dma_start(out=mask_sb, in_=maskT[:, :])
        eye = cross.tile([128, 128], BF, tag="eye")
        nc.sync.dma_start(out=eye, in_=eyed[:, :])
        eps5 = cross.tile([128, 1], F32, tag="eps5")
        nc.vector.memset(eps5, 1e-5)
        eps24 = cross.tile([128, 1], F32, tag="eps24")
        nc.vector.memset(eps24, 1e-24)
        ctxF = cross.tile([128, KD, NTOK], BF, tag="ctxF")

        # ================= ENCODER SCOPE =================
        with tc.tile_pool(name="wenc", bufs=1) as wenc, \
             tc.tile_pool(name="bige", bufs=1) as bige, \
             tc.tile_pool(name="sme", bufs=1) as sme, \
             tc.tile_pool(name="pmm", bufs=6, space="PSUM") as pmm, \
             tc.tile_pool(name="pbc", bufs=1, space="PSUM") as pbc:

            def ln_fm(x_sb, g_ap, b_ap, out, l2_out=None, triv=False):
                """x_sb [128, KD, 1024] bf16 -> LN -> out; optional l2 -> l2_out.
                triv: gamma==1 and beta==0 (runtime-specialized)."""
                for j in range(2):
                    jj = slice(j * 512, (j + 1) * 512)
                    s12 = pbc.tile([128, 2, 512], F32, tag="pabc", bufs=1,
                                   name="s12")[:1]
                    for k in range(KD):
                        sq = bige.tile([128, 512], BF, tag="sqc", bufs=2)
                        nc.vector.tensor_mul(sq, x_sb[:, k, jj], x_sb[:, k, jj])
                        nc.tensor.matmul(s12[:, 0, :], ones_col, x_sb[:, k, jj],
                                         start=(k == 0), stop=(k == KD - 1))
                        nc.tensor.matmul(s12[:, 1, :], ones_col, sq,
                                         start=(k == 0), stop=(k == KD - 1))
                    m = sme.tile([1, 512], F32, tag="mtag", bufs=2)
                    nc.vector.tensor_scalar_mul(m, s12[:, 0, :], 1.0 / D)
                    msq = sme.tile([1, 512], F32, tag="scr512", bufs=2)
                    nc.vector.tensor_mul(msq, m, m)
                    var = sme.tile([1, 512], F32, tag="scr512", bufs=2)
                    nc.vector.scalar_tensor_tensor(var, s12[:, 1, :], 1.0 / D,
                                                   msq, OP.mult, OP.subtract)
                    sd = sme.tile([1, 512], F32, tag="scr512", bufs=2)
                    nc.scalar.activation(sd, var, AF.Sqrt, bias=eps5[:1, :])
                    ac = sme.tile([1, 2, 512], F32, tag="acf")
                    nc.vector.reciprocal(ac[:, 0, :], sd)
                    nc.vector.scalar_tensor_tensor(ac[:, 1, :], m, -1.0,
                                                   ac[:, 0, :], OP.mult, OP.mult)
                    pac = pbc.tile([128, 2, 512], F32, tag="pabc", bufs=1)
                    for a in range(2):
                        nc.tensor.matmul(pac[:, a, :], ones_row_f, ac[:, a, :],
                                         start=True, stop=True)
                    pab = bige.tile([128, 2, 512], BF, tag="pab16", bufs=2)
                    nc.scalar.activation(pab, pac, AF.Copy)
                    for k in range(KD):
                        if triv:
                            t1 = bige.tile([128, 512], BF, tag="t1", bufs=2)
                            nc.vector.tensor_mul(t1, x_sb[:, k, jj], pab[:, 0, :])
                            nc.vector.tensor_add(out[:, k, jj], t1, pab[:, 1, :])
                        else:
                            t1 = bige.tile([128, 512], F32, tag="t1f", bufs=2)
                            nc.vector.tensor_mul(t1, x_sb[:, k, jj], pab[:, 0, :])
                            nc.vector.tensor_add(t1, t1, pab[:, 1, :])
                            nc.vector.tensor_scalar(out[:, k, jj], t1,
                                                    g_ap[:, k:k + 1],
                                                    b_ap[:, k:k + 1],
                                                    OP.mult, OP.add)
                if l2_out is None:
                    return
                for j in range(2):
                    jj = slice(j * 512, (j + 1) * 512)
                    s1t = pbc.tile([128, 2, 512], F32, tag="pabc", bufs=1,
                                   name="s1t")[:1]
                    s1 = s1t[:, 0, :]
                    for k in range(KD):
                        sq = bige.tile([128, 512], BF, tag="sqc", bufs=2)
                        nc.vector.tensor_mul(sq, out[:, k, jj], out[:, k, jj])
                        nc.tensor.matmul(s1, ones_col, sq,
                                         start=(k == 0), stop=(k == KD - 1))
                    sd = sme.tile([1, 512], F32, tag="scr512", bufs=2)
                    nc.scalar.activation(sd, s1, AF.Sqrt, bias=eps24[:1, :])
                    rr = sme.tile([1, 512], F32, tag="scr512", bufs=2)
                    nc.vector.reciprocal(rr, sd)
                    pat = pbc.tile([128, 2, 512], F32, tag="pabc", bufs=1)
                    pa = pat[:, 0, :]
                    nc.tensor.matmul(pa, ones_row_f, rr, start=True, stop=True)
                    pa16 = bige.tile([128, 512], BF, tag="pa16", bufs=2)
                    nc.scalar.activation(pa16, pa, AF.Copy)
                    for k in range(KD):
                        nc.vector.tensor_mul(l2_out[:, k, jj], out[:, k, jj], pa16)

            X = bige.tile([128, KD, NTOK], BF, tag="X")
            nc.sync.dma_start(out=X, in_=r3(x0T))
            gA = sme.tile([128, NL, KD], F32, tag="gA")
            bA = sme.tile([128, NL, KD], F32, tag="bA")
            gB = sme.tile([128, NL, KD], F32, tag="gB")
            bB = sme.tile([128, NL, KD], F32, tag="bB")
            for l in range(NL):
                nc.sync.dma_start(out=gA[:, l, :], in_=enc_in[f"g1{l}"][:, :])
                nc.sync.dma_start(out=bA[:, l, :], in_=enc_in[f"bb1{l}"][:, :])
                nc.sync.dma_start(out=gB[:, l, :], in_=enc_in[f"g2{l}"][:, :])
                nc.sync.dma_start(out=bB[:, l, :], in_=enc_in[f"bb2{l}"][:, :])

            for l in range(NL):
                XN = bige.tile([128, KD, NTOK], BF, tag="XN")
                ln_fm(X, gA[:, l, :], bA[:, l, :], XN, triv=LN_TRIV)
                wqk = wenc.tile([128, KD, 2048], BF, tag="wbig")
                nc.sync.dma_start(out=wqk, in_=r3(enc_in[f"wqk{l}"]))
                bqk = sme.tile([128, 16], F32, tag="bqk")
                nc.sync.dma_start(out=bqk, in_=enc_in[f"bqk{l}"][:, :])
                # v projection (token-major)
                wv = wenc.tile([128, KD, D], BF, tag="wsmall")
                nc.sync.dma_start(out=wv, in_=r3(enc_in[f"wv{l}"]))
                if not BV0:
                    ones_row = sme.tile([1, 128], BF, tag="ones_row")
                    nc.vector.memset(ones_row, 1.0)
                    bvr = sme.tile([1, D], BF, tag="bvr")
                    nc.sync.dma_start(out=bvr, in_=enc_in[f"bv{l}"][:, :])
                    bvs = bige.tile([128, D], BF, tag="bvs")
                    for j in range(2):
                        pb = pbc.tile([128, 2, 512], F32, tag="pabc", bufs=1)
                        nc.tensor.matmul(pb[:, 0, :384], ones_row,
                                         bvr[:, j * 384:(j + 1) * 384],
                                         start=True, stop=True)
                        nc.vector.tensor_copy(bvs[:, j * 384:(j + 1) * 384],
                                              pb[:, 0, :384])
                vS = bige.tile([64, B, D], BF, tag="vS")
                for b in range(B):
                    for j in range(2):
                        jj = slice(j * 384, (j + 1) * 384)
                        ps = pmm.tile([128, 512], F32, tag="pmm", name="psv")
                        for k in range(KD):
                            nc.tensor.matmul(ps[:64, :384],
                                             XN[:, k, b * 64:(b + 1) * 64],
                                             wv[:, k, jj],
                                             start=(k == 0), stop=(k == KD - 1))
                        if not BV0:
                            nc.vector.tensor_add(vS[:, b, jj], ps[:64, :384],
                                                 bvs[:64, jj])
                        elif (2 * b + j) % 2 == 0:
                            nc.scalar.activation(vS[:, b, jj], ps[:64, :384],
                                                 AF.Copy)
                        else:
                            nc.vector.tensor_copy(vS[:, b, jj], ps[:64, :384])
                # attention per head; additive mask injected into score PSUM
                ofS = bige.tile([128, NH, NTOK], BF, tag="ofS")
                nc.vector.memset(ofS[96:128, :, :], 0.0)
                for h in range(NH):
                    qh = bige.tile([128, NTOK], BF, tag="qh", bufs=2)
                    kh = bige.tile([128, NTOK], BF, tag="kh", bufs=2)
                    for j in range(2):
                        jj = slice(j * 512, (j + 1) * 512)
                        ph = pmm.tile([128, 512], F32, tag="pmm", name="phq")
                        for k in range(KD):
                            nc.tensor.matmul(ph,
                                             wqk[:, k, h * 128:(h + 1) * 128],
                                             XN[:, k, jj],
                                             start=(k == 0), stop=(k == KD - 1))
                        if (h + j) % 2 == 0:
                            nc.scalar.activation(qh[:, jj], ph, AF.Identity,
                                                 bias=bqk[:, h:h + 1])
                        else:
                            nc.vector.tensor_scalar_add(qh[:, jj], ph,
                                                        bqk[:, h:h + 1])
                        ph2 = pmm.tile([128, 512], F32, tag="pmm", name="phk")
                        for k in range(KD):
                            nc.tensor.matmul(ph2,
                                             wqk[:, k, 1024 + h * 128:1024 + (h + 1) * 128],
                                             XN[:, k, jj],
                                             start=(k == 0), stop=(k == KD - 1))
                        if (h + j) % 2 == 0:
                            nc.vector.tensor_scalar_add(kh[:, jj], ph2,
                                                        bqk[:, 8 + h:9 + h])
                        else:
                            nc.scalar.activation(kh[:, jj], ph2, AF.Identity,
                                                 bias=bqk[:, 8 + h:9 + h])
                    eT = bige.tile([64, NTOK], BF, tag="eT", bufs=3)
                    for j in range(2):
                        jj = slice(j * 512, (j + 1) * 512)
                        sT = pmm.tile([128, 512], F32, tag="pmm", name="sT")
                        for b in range(8 * j, 8 * j + 8):
                            c = slice((b % 8) * 64, (b % 8) * 64 + 64)
                            nc.tensor.matmul(sT[:64, c],
                                             kh[:, b * 64:(b + 1) * 64],
                                             qh[:, b * 64:(b + 1) * 64],
                                             start=True, stop=False)
                            nc.tensor.matmul(sT[:64, c],
                                             eye[:64, :64], mask_sb,
                                             start=False, stop=True)
                        nc.scalar.activation(eT[:, jj], sT[:64, :], AF.Exp,
                                             scale=QK_SCALE)
                    ssum = pbc.tile([128, 2, 512], F32, tag="pabc", bufs=1,
                                    name="ssum")[:1]
                    for a in range(2):
                        nc.tensor.matmul(ssum[:, a, :], ones64,
                                         eT[:, a * 512:(a + 1) * 512],
                                         start=True, stop=True)
                    rs2 = sme.tile([1, 2, 512], F32, tag="acf")
                    rs = rs2[:, :, :].rearrange("p a c -> p (a c)")
                    nc.vector.reciprocal(rs, ssum[:, :, :].rearrange("p a c -> p (a c)"))
                    eN = bige.tile([64, NTOK], BF, tag="eN", bufs=3)
                    for a in range(2):
                        aa = slice(a * 512, (a + 1) * 512)
                        rb64 = pmm.tile([128, 512], F32, tag="pmm", name="rb64")
                        nc.tensor.matmul(rb64[:64, :], ones_r64f, rs[:, aa],
                                         start=True, stop=True)
                        nc.vector.tensor_mul(eN[:, aa], eT[:, aa], rb64[:64, :])
                    for j in range(2):
                        jj = slice(j * 512, (j + 1) * 512)
                        oh = pmm.tile([128, 512], F32, tag="pmm", name="oh")
                        for b in range(8 * j, 8 * j + 8):
                            c = slice((b % 8) * 64, (b % 8) * 64 + 64)
                            nc.tensor.matmul(oh[:HD, c],
                                             vS[:, b, h * HD:(h + 1) * HD],
                                             eN[:, b * 64:(b + 1) * 64],
                                             start=True, stop=True)
                        nc.scalar.activation(ofS[:HD, h, jj], oh[:HD, :], AF.Copy)
                # attn out + residual (in place on X)
                wao = wenc.tile([128, NH, D], BF, tag="wsmall")
                nc.sync.dma_start(out=wao,
                                  in_=enc_in[f"wao{l}"][:, :].rearrange("(k p) n -> p k n", p=128))
                bao = sme.tile([128, KD], F32, tag="bao")
                nc.sync.dma_start(out=bao, in_=enc_in[f"bao{l}"][:, :])
                for mo in range(KD):
                    for j in range(2):
                        jj = slice(j * 512, (j + 1) * 512)
                        ps = pmm.tile([128, 512], F32, tag="pmm", name="pswo")
                        for kv in range(NH):
                            nc.tensor.matmul(ps,
                                             wao[:, kv, mo * 128:(mo + 1) * 128],
                                             ofS[:, kv, jj],
                                             start=(kv == 0), stop=(kv == NH - 1))
                        if B2Z:
                            nc.vector.tensor_add(X[:, mo, jj], X[:, mo, jj], ps)
                        else:
                            t2 = bige.tile([128, 512], F32, tag="t2", bufs=2)
                            nc.vector.tensor_scalar_add(t2, ps, bao[:, mo:mo + 1])
                            nc.vector.tensor_add(X[:, mo, jj], X[:, mo, jj], t2)
                # ffn
                XN2 = bige.tile([128, KD, NTOK], BF, tag="XN")
                ln_fm(X, gB[:, l, :], bB[:, l, :], XN2, triv=LN_TRIV)
                w1 = wenc.tile([128, KD, FFN], BF, tag="wbig")
                nc.sync.dma_start(out=w1, in_=r3(enc_in[f"ff1T{l}"]))
                bf1 = sme.tile([128, MF], F32, tag="bf1")
                nc.sync.dma_start(out=bf1, in_=enc_in[f"bff1{l}"][:, :])
                G = bige.tile([128, MF, NTOK], BF, tag="G")
                for mf in range(MF):
                    for j in range(2):
                        jj = slice(j * 512, (j + 1) * 512)
                        ps = pmm.tile([128, 512], F32, tag="pmm", name="psf1")
                        for k in range(KD):
                            nc.tensor.matmul(ps,
                                             w1[:, k, mf * 128:(mf + 1) * 128],
                                             XN2[:, k, jj],
                                             start=(k == 0), stop=(k == KD - 1))
                        nc.scalar.activation(G[:, mf, jj], ps, AF.Gelu,
                                             bias=bf1[:, mf:mf + 1])
                w2 = wenc.tile([128, MF, D], BF, tag="wbig")
                nc.sync.dma_start(out=w2, in_=r3(enc_in[f"ff2T{l}"]))
                bf2 = sme.tile([128, KD], F32, tag="bao")
                nc.sync.dma_start(out=bf2, in_=enc_in[f"bff2{l}"][:, :])
                for mo in range(KD):
                    for j in range(2):
                        jj = slice(j * 512, (j + 1) * 512)
                        ps = pmm.tile([128, 512], F32, tag="pmm", name="psf2")
                        for kf in range(MF):
                            nc.tensor.matmul(ps,
                                             w2[:, kf, mo * 128:(mo + 1) * 128],
                                             G[:, kf, jj],
                                             start=(kf == 0), stop=(kf == MF - 1))
                        if B2Z:
                            nc.vector.tensor_add(X[:, mo, jj], X[:, mo, jj], ps)
                        else:
                            t2 = bige.tile([128, 512], F32, tag="t2", bufs=2)
                            nc.vector.tensor_scalar_add(t2, ps, bf2[:, mo:mo + 1])
                            nc.vector.tensor_add(X[:, mo, jj], X[:, mo, jj], t2)

            # final norm + l2 -> ctxF (cross pool)
            eg = sme.tile([128, KD], F32, tag="eg")
            eb = sme.tile([128, KD], F32, tag="eb")
            nc.sync.dma_start(out=eg, in_=encg[:, :])
            nc.sync.dma_start(out=eb, in_=encb[:, :])
            pre = bige.tile([128, KD, NTOK], BF, tag="XN")
            ln_fm(X, eg, eb, pre, l2_out=ctxF, triv=LN_TRIV)


        # ================= RECURRENCE SCOPE =================
        ctx_r = ctxF[:, :, :].rearrange("p k (b t) -> p t k b", b=B)
        with tc.tile_pool(name="wrec", bufs=1) as wrec, \
             tc.tile_pool(name="smr", bufs=2) as smr, \
             tc.tile_pool(name="prc", bufs=1, space="PSUM") as prc, \
             tc.tile_pool(name="pr6", bufs=1, space="PSUM") as pr6, \
             tc.tile_pool(name="prs", bufs=1, space="PSUM") as prs, \
             tc.tile_pool(name="pou", bufs=1, space="PSUM") as pou:

            Whu = wrec.tile([128, KD, HID], BF, tag="Whu")
            nc.sync.dma_start(out=Whu, in_=r3(Whud))
            V0w = wrec.tile([128, KD, HID], BF, tag="V0w")
            nc.sync.dma_start(out=V0w, in_=r3(V0d))
            gw1w = wrec.tile([128, KD, D], BF, tag="gw1w")
            nc.sync.dma_start(out=gw1w, in_=r3(gw1d))
            V1w = wrec.tile([128, MH, D], BF, tag="V1w")
            nc.sync.dma_start(out=V1w, in_=r3(V1d))
            Wtf = wrec.tile([128, KD, D], BF, tag="Wtf")
            nc.sync.dma_start(out=Wtf, in_=r3(Wtfd))
            cw1 = wrec.tile([128, KD, D], BF, tag="cw1")
            nc.sync.dma_start(out=cw1, in_=r3(cw1d))
            cw2 = wrec.tile([128, KD, D], BF, tag="cw2")
            nc.sync.dma_start(out=cw2, in_=r3(cw2d))
            gw2 = wrec.tile([128, KD, D], BF, tag="gw2")
            nc.sync.dma_start(out=gw2, in_=r3(gw2d))
            b1S = smr.tile([128, KD], F32, tag="b1S", bufs=1)
            nc.sync.dma_start(out=b1S, in_=b1d[:, :])
            cb1S = smr.tile([128, KD], F32, tag="cb1S", bufs=1)
            nc.sync.dma_start(out=cb1S, in_=cb1d[:, :])
            cb2S = smr.tile([128, KD], F32, tag="cb2S", bufs=1)
            nc.sync.dma_start(out=cb2S, in_=cb2d[:, :])
            # bf16 broadcast copies (B cols) for PSUM bias injection
            b0S = smr.tile([128, MH], F32, tag="b0S", bufs=1)
            nc.sync.dma_start(out=b0S, in_=b0d[:, :])
            b0b = smr.tile([128, MH, B], BF, tag="b0b", bufs=1)
            nc.vector.tensor_copy(b0b, bc(b0S[:, :], B, 2))
            gbS = smr.tile([128, KD], F32, tag="gbS", bufs=1)
            nc.sync.dma_start(out=gbS, in_=gbd[:, :])
            gbb = smr.tile([128, KD, B], BF, tag="gbb", bufs=1)
            nc.vector.tensor_copy(gbb, bc(gbS[:, :], B, 2))
            b1b = smr.tile([128, KD, B], BF, tag="b1b", bufs=1)
            nc.vector.tensor_copy(b1b, bc(b1S[:, :], B, 2))
            cb1b = smr.tile([128, KD, B], BF, tag="cb1b", bufs=1)
            nc.vector.tensor_copy(cb1b, bc(cb1S[:, :], B, 2))
            cb2b = smr.tile([128, KD, B], BF, tag="cb2b", bufs=1)
            nc.vector.tensor_copy(cb2b, bc(cb2S[:, :], B, 2))
            outgS = smr.tile([128, KD], F32, tag="outgS", bufs=1)
            nc.sync.dma_start(out=outgS, in_=outgd[:, :])
            outbS = smr.tile([128, KD], F32, tag="outbS", bufs=1)
            nc.sync.dma_start(out=outbS, in_=outbd[:, :])
            Hc = [wrec.tile([128, KD, 128], BF, tag=f"H{mt}", name=f"H{mt}")
                  for mt in range(MT)]

            import os as _os
            T_RUN = int(_os.environ.get("T_TRUNC", T))
            if T_RUN < T:
                for Hcm in Hc:
                    nc.vector.memset(Hcm, 0.0)

            NBLK = (T_RUN + 7) // 8
            if KDEBUG:
                nc.sync.dma_start(out=dbg_ctx[:, :],
                                  in_=ctxF[:, :, :].rearrange("p k n -> p (k n)"))

            # lm chunk emission: during step t of block w, process chunks of
            # block w-1 (Hc[w-1] is complete); 13 chunks spread over 8 steps.
            LM_SPLIT = [0, 2, 4, 6, 8, 10, 11, 12, 13]

            def lm_chunks(mt, j0, j1):
                for j in range(j0, j1):
                    lw = wrec.tile([128, KD, 512], BF, tag="lmw", bufs=2)
                    nc.sync.dma_start(
                        out=lw,
                        in_=lmTd[:, j * 512:(j + 1) * 512].rearrange("(k p) n -> p k n",
                                                                     p=128))
                    ps = pou.tile([128, 512], F32, tag="plm", bufs=2)
                    for k in range(KD):
                        nc.tensor.matmul(ps, Hc[mt][:, k, :], lw[:, k, :],
                                         start=(k == 0), stop=(k == KD - 1))
                    ot = smr.tile([128, 512], BF, tag="otile", bufs=3)
                    if j % 2 == 0:
                        nc.scalar.activation(ot, ps, AF.Copy)
                    else:
                        nc.vector.tensor_copy(ot, ps)
                    nc.sync.dma_start(out=outd[mt * 128:(mt + 1) * 128,
                                               j * 512:(j + 1) * 512], in_=ot)

            u_tiles = {}
            gp_tiles = {}

            def emit_u_prefix(tt, mlo=0, mhi=MH):
                # b0 + V0^T ctx accumulation prefix for step tt (h-independent),
                # emitted in chunks interleaved with Pool-LN calls so the PE
                # runs it during those idle windows. One psum bank: single
                # start (zeroes whole 2KB region), single stop at group end.
                if mlo == 0:
                    u_tiles[tt] = prc.tile([128, 32, B], F32, tag="pu", bufs=2,
                                           name=f"u{tt}")[:, :MH, :]
                un = u_tiles[tt]
                cx = ctx_r[:, tt]
                for m in range(mlo, mhi):
                    nc.tensor.matmul(un[:, m, :], eye, b0b[:, m, :],
                                     start=(m == 0), stop=False)
                    for k in range(KD):
                        nc.tensor.matmul(un[:, m, :], V0w[:, k, m * 128:(m + 1) * 128],
                                         cx[:, k, :], start=False,
                                         stop=(tt == 0 and m == MH - 1 and k == KD - 1))

            def emit_gp_prefix(tt, mlo=0, mhi=KD):
                if mlo == 0:
                    gp_tiles[tt] = pr6.tile([128, 8, B], F32, tag="pgp", bufs=2,
                                            name=f"gp{tt}")[:, :KD, :]
                gpn = gp_tiles[tt]
                cx = ctx_r[:, tt]
                for m in range(mlo, mhi):
                    nc.tensor.matmul(gpn[:, m, :], eye, gbb[:, m, :],
                                     start=(m == 0), stop=False)
                    for k in range(KD):
                        nc.tensor.matmul(gpn[:, m, :],
                                         gw1w[:, k, m * 128:(m + 1) * 128],
                                         cx[:, k, :], start=False, stop=False)

            emit_u_prefix(0)
            emit_gp_prefix(0)
            for t in range(T_RUN):
                ctx_t = ctx_r[:, t]
                mtb, off = t // 8, (t % 8) * B
                if t > 0:
                    hprev = Hc[(t - 1) // 8][:, :, ((t - 1) % 8) * B:((t - 1) % 8) * B + B]
                u = u_tiles.pop(t)
                if t > 0:
                    for half in range(2):
                        cc = slice(8 * half, 8 * half + 8)
                        for m in range(MH):
                            for k in range(KD):
                                nc.tensor.matmul(
                                    u[:, m, cc], Whu[:, k, m * 128:(m + 1) * 128],
                                    hprev[:, k, cc], start=False,
                                    stop=(half == 1 and m == MH - 1 and k == KD - 1))
                hsS = smr.tile([128, MH, B], BF, tag="hsS")
                nc.scalar.activation(hsS, u, AF.Gelu)
                # tf branch (off critical path): w = tf - ctx
                w_t = smr.tile([128, KD, B], F32, tag="w_t")
                if t > 0:
                    tfp = prc.tile([128, 8, B], F32, tag="ptf", name="tfp")[:, :KD, :]
                    for m in range(KD):
                        for k in range(KD):
                            nc.tensor.matmul(tfp[:, m, :], Wtf[:, k, m * 128:(m + 1) * 128],
                                             hprev[:, k, :], start=(k == 0),
                                             stop=(k == KD - 1))
                    nc.vector.tensor_sub(w_t, tfp, ctx_t)
                else:
                    nc.vector.tensor_scalar_mul(w_t, ctx_t, -1.0)
                cpr = pr6.tile([128, 8, B], F32, tag="p6", name="cpr")[:, :KD, :]
                for m in range(KD):
                    for k in range(MH):
                        nc.tensor.matmul(cpr[:, m, :], V1w[:, k, m * 128:(m + 1) * 128],
                                         hsS[:, k, :], start=(k == 0), stop=False)
                    nc.tensor.matmul(cpr[:, m, :], eye, b1b[:, m, :],
                                     start=False, stop=True)
                # cpf stored b-major so per-b layernorm views are contiguous
                # (gpsimd firmware requires contiguous [128, F] rows)
                cpf = smr.tile([128, B, KD], F32, tag="cpf")
                for half in range(2):
                    cc = slice(8 * half, 8 * half + 8)
                    nc.scalar.activation(
                        cpf[:, cc, :].rearrange("p b k -> p k b"),
                        cpr[:, :, cc], AF.Gelu)
                # l2norm via gpsimd rmsnorm (no act-table); 1/sqrt(D) folded
                # into cw1 host-side.
                corePool = smr.tile([128, B, KD], F32, tag="corePool")
                coreS = smr.tile([128, KD, B], BF, tag="coreS")
                gm = pr6.tile([128, 8, B], F32, tag="p6", name="gm")[:, :KD, :]
                for b in range(B):
                    nc.gpsimd.layernorm(corePool[:, b, :], cpf[:, b, :],
                                        subtract_mean=False, eps=1e-10)
                    if b % 4 == 3 and t + 1 < T_RUN:
                        emit_u_prefix(t + 1, 6 * (b // 4), 6 * (b // 4 + 1))
                    if b % 8 == 7:
                        # half-batch: copy + start cw1 while the other half's
                        # rmsnorm calls still run on Pool
                        half = b // 8
                        cc = slice(8 * half, 8 * half + 8)
                        nc.vector.tensor_copy(
                            coreS[:, :, cc],
                            corePool[:, cc, :].rearrange("p b k -> p k b"))
                        if half == 0:
                            for m in range(KD):
                                nc.tensor.matmul(gm[:, m, :], eye, cb1b[:, m, :],
                                                 start=(m == 0), stop=False)
                        for m in range(KD):
                            for k in range(KD):
                                nc.tensor.matmul(
                                    gm[:, m, cc],
                                    cw1[:, k, m * 128:(m + 1) * 128],
                                    coreS[:, k, cc], start=False,
                                    stop=(half == 1 and m == KD - 1 and k == KD - 1))
                gmS = smr.tile([128, KD, B], BF, tag="gmS")
                nc.scalar.activation(gmS, gm, AF.Gelu)
                cfp = pr6.tile([128, 8, B], F32, tag="p6", name="cfp")[:, :KD, :]
                for m in range(KD):
                    for k in range(KD):
                        nc.tensor.matmul(cfp[:, m, :], cw2[:, k, m * 128:(m + 1) * 128],
                                         gmS[:, k, :], start=(k == 0), stop=False)
                    nc.tensor.matmul(cfp[:, m, :], eye, cb2b[:, m, :],
                                     start=False, stop=True)
                cfF = smr.tile([128, KD, B], F32, tag="cfF")
                nc.scalar.activation(cfF, cfp, AF.Copy)
                # gate cf-branch folded through cp_w2: reads g1 (gmS) directly,
                # in parallel with the cfp/cfF branch
                gp = gp_tiles.pop(t)
                for m in range(KD):
                    for k in range(KD):
                        nc.tensor.matmul(gp[:, m, :], gw2[:, k, m * 128:(m + 1) * 128],
                                         gmS[:, k, :], start=False,
                                         stop=(m == KD - 1 and k == KD - 1))
                # gate via tanh (shares the gelu act table):
                #   sigmoid(x) = 0.5*(1 + tanh(x/2))
                #   hp = gate*(cf+tf-ctx) + ctx = 0.5*(tanh+1)*(cf+tf-ctx) + ctx
                a1 = smr.tile([128, KD, B], F32, tag="a1")
                nc.vector.tensor_add(a1, cfF, w_t)
                tG = smr.tile([128, KD, B], F32, tag="gateS")
                q1 = smr.tile([128, KD, B], F32, tag="q1")
                hp = smr.tile([128, B, KD], F32, tag="hp")
                for half in range(2):
                    cc = slice(8 * half, 8 * half + 8)
                    nc.scalar.activation(tG[:, :, cc], gp[:, :, cc], AF.Tanh,
                                         scale=0.5)
                    nc.vector.scalar_tensor_tensor(q1[:, :, cc], tG[:, :, cc], 1.0,
                                                   a1[:, :, cc], OP.add, OP.mult)
                    nc.vector.scalar_tensor_tensor(
                        hp[:, cc, :].rearrange("p b k -> p k b"),
                        q1[:, :, cc], 0.5, ctx_t[:, :, cc], OP.mult, OP.add)
                # LN + affine via gpsimd layernorm (no act-table), then clip
                # halves so next step's Whu starts under the second LN half.
                lnout = smr.tile([128, B, KD], F32, tag="lnout")
                for b in range(B):
                    nc.gpsimd.layernorm(lnout[:, b, :], hp[:, b, :],
                                        gamma_ap=outgS[:, :], beta_ap=outbS[:, :],
                                        subtract_mean=True, eps=1e-5)
                    if b % 8 == 7:
                        half = b // 8
                        cc = slice(8 * half, 8 * half + 8)
                        nc.vector.tensor_scalar(
                            Hc[t // 8][:, :, (t % 8) * B + 8 * half:
                                       (t % 8) * B + 8 * half + 8],
                            lnout[:, cc, :].rearrange("p b k -> p k b"),
                            5.0, -5.0, OP.min, OP.max)
                        if t + 1 < T_RUN:
                            emit_gp_prefix(t + 1, 3 * half, 3 * (half + 1))
                if KDEBUG and t == min(7, T_RUN - 1):
                    nc.sync.dma_start(out=dbg_h[:, :],
                                      in_=Hc[0][:, :, :].rearrange("p k c -> p (k c)"))
                if KDEBUG and t == int(_os.environ.get("KPROBE_T", 0)):
                    for nm, tile_ in [("hs", hsS), ("cpf", cpf), ("core", coreS),
                                      ("gm", gmS), ("cf", cfF), ("tg", tG),
                                      ("hp", hp), ("ln", lnout)]:
                        nc.sync.dma_start(
                            out=dbg_st[nm][:, :],
                            in_=tile_[:, :, :].rearrange("p a b -> p (a b)"))
                # interleave lm-head work for the previous (complete) block
                if t >= 8 and T_RUN == T:
                    lm_chunks(mtb - 1, LM_SPLIT[t % 8], LM_SPLIT[t % 8 + 1])

            # lm-head tail: last block (or all blocks on truncated debug runs)
            tail_blocks = range(NBLK - 1, NBLK) if T_RUN == T else range(NBLK)
            for mt in tail_blocks:
                lm_chunks(mt, 0, NCH)

    nc.finalize()
    return nc


_NC_CACHE = {}


def _get_nc():
    key = (_FLAGS["ln_triv"], _FLAGS["bv0"], _FLAGS["b2z"])
    if key not in _NC_CACHE:
        _NC_CACHE[key] = build_nc()
    return _NC_CACHE[key]


def _prep_inputs(inputs):
    f = lambda x: np.asarray(x, np.float32)
    tok = np.asarray(inputs["token_ids"]).astype(np.int64)
    emb, pos = f(inputs["emb"]), f(inputs["pos_emb"])
    x0 = emb[tok.reshape(-1)] + np.tile(pos[:T], (B, 1))
    com = {"x0T": x0.T.astype(bfnp)}
    aiw, aib = f(inputs["attn_in_w"]), f(inputs["attn_in_b"])
    aow, aob = f(inputs["attn_out_w"]), f(inputs["attn_out_b"])
    for l in range(NL):
        wqk = np.zeros((D, 2048), np.float32)
        bqk = np.zeros(2048, np.float32)
        wv = np.zeros((D, D), np.float32)
        bv = np.zeros(D, np.float32)
        wao = np.zeros((1024, D), np.float32)
        for h in range(NH):
            wqk[:, h * 128:h * 128 + HD] = aiw[l, h * HD:(h + 1) * HD, :].T
            wqk[:, 1024 + h * 128:1024 + h * 128 + HD] = aiw[l, D + h * HD:D + (h + 1) * HD, :].T
            bqk[h * 128:h * 128 + HD] = aib[l, h * HD:(h + 1) * HD]
            bqk[1024 + h * 128:1024 + h * 128 + HD] = aib[l, D + h * HD:D + (h + 1) * HD]
            wv[:, h * HD:(h + 1) * HD] = aiw[l, 2 * D + h * HD:2 * D + (h + 1) * HD, :].T
            bv[h * HD:(h + 1) * HD] = aib[l, 2 * D + h * HD:2 * D + (h + 1) * HD]
            wao[h * 128:h * 128 + HD, :] = aow[l][:, h * HD:(h + 1) * HD].T
        com[f"wqk{l}"] = wqk.astype(bfnp)
        com[f"bqk{l}"] = bqk.reshape(16, 128).T.copy()
        com[f"wv{l}"] = wv.astype(bfnp)
        com[f"bv{l}"] = bv.reshape(1, D).astype(bfnp)
        com[f"wao{l}"] = wao.astype(bfnp)
        com[f"bao{l}"] = aob[l].reshape(KD, 128).T.copy()
        com[f"ff1T{l}"] = f(inputs["ff_w1"])[l].T.astype(bfnp).copy()
        com[f"bff1{l}"] = f(inputs["ff_b1"])[l].reshape(MF, 128).T.copy()
        com[f"ff2T{l}"] = f(inputs["ff_w2"])[l].T.astype(bfnp).copy()
        com[f"bff2{l}"] = f(inputs["ff_b2"])[l].reshape(KD, 128).T.copy()
        com[f"g1{l}"] = f(inputs["n1_g"])[l].reshape(KD, 128).T.copy()
        com[f"bb1{l}"] = f(inputs["n1_b"])[l].reshape(KD, 128).T.copy()
        com[f"g2{l}"] = f(inputs["n2_g"])[l].reshape(KD, 128).T.copy()
        com[f"bb2{l}"] = f(inputs["n2_b"])[l].reshape(KD, 128).T.copy()
    com["encg"] = f(inputs["enc_norm_g"]).reshape(KD, 128).T.copy()
    com["encb"] = f(inputs["enc_norm_b"]).reshape(KD, 128).T.copy()
    tk, tq = np.meshgrid(np.arange(64), np.arange(64), indexing="ij")
    com["maskT"] = ((tk > tq) * -30000.0).astype(bfnp)
    com["eyed"] = np.eye(128, dtype=bfnp)
    com["V0d"] = f(inputs["V0"]).astype(bfnp)
    com["V1d"] = f(inputs["V1"]).astype(bfnp)
    R, tw = f(inputs["R"]), f(inputs["temp_w"])
    # folded recurrent weights: u += (alpha*R@V0)^T h ; tf = (alpha*R@tw^T)^T h
    com["Whud"] = (ALPHA * R @ f(inputs["V0"])).astype(bfnp)
    com["Wtfd"] = (ALPHA * R @ tw.T).astype(bfnp)
    # rmsnorm(x) = sqrt(D) * l2norm(x): fold the 1/sqrt(D) into cp_w1
    com["cw1d"] = (f(inputs["cp_w1"]).T / np.sqrt(D)).astype(bfnp).copy()
    com["cw2d"] = f(inputs["cp_w2"]).T.astype(bfnp).copy()
    gw = f(inputs["gate_w"])
    com["gw1d"] = gw[:, :D].T.astype(bfnp).copy()
    # gate cf-branch folded through cp_w2: gs_cf = Wgg^T g1 (+ gwB@cp_b2 -> gb)
    gwB = gw[:, D:]
    com["gw2d"] = np.ascontiguousarray((gwB @ f(inputs["cp_w2"])).T).astype(bfnp)
    com["gbd"] = (f(inputs["gate_b"]) + gwB @ f(inputs["cp_b2"])).reshape(KD, 128).T.copy()
    com["b0d"] = f(inputs["b0"]).reshape(MH, 128).T.copy()
    com["b1d"] = f(inputs["b1"]).reshape(KD, 128).T.copy()
    com["cb1d"] = f(inputs["cp_b1"]).reshape(KD, 128).T.copy()
    com["cb2d"] = f(inputs["cp_b2"]).reshape(KD, 128).T.copy()
    com["outgd"] = np.ascontiguousarray(f(inputs["out_g"]).reshape(KD, 128).T)
    com["outbd"] = np.ascontiguousarray(f(inputs["out_b"]).reshape(KD, 128).T)
    lmp = np.zeros((VP, D), np.float32)
    lmp[:V] = f(inputs["lm_head"])
    lmT = lmp.T.astype(bfnp)
    shards = [np.ascontiguousarray(lmT[:, c * VS:(c + 1) * VS]) for c in range(8)]
    return com, shards


LAST_RESULT = {}


def kernel(**inputs):
    import os
    f = lambda x: np.asarray(x, np.float32)
    _FLAGS["ln_triv"] = bool(
        all(np.all(f(inputs[k]) == 1.0) for k in ("n1_g", "n2_g", "enc_norm_g"))
        and all(np.all(f(inputs[k]) == 0.0) for k in ("n1_b", "n2_b", "enc_norm_b")))
    _FLAGS["bv0"] = bool(np.all(f(inputs["attn_in_b"])[:, 2 * D:] == 0.0))
    _FLAGS["b2z"] = bool(np.all(f(inputs["attn_out_b"]) == 0.0)
                         and np.all(f(inputs["ff_b2"]) == 0.0))
    nc = _get_nc()
    com, shards = _prep_inputs(inputs)
    in_maps = [{**com, "lmTd": shards[c]} for c in range(8)]
    kw = {}
    if os.environ.get("KTRACE"):
        kw = dict(trace=True, tmpdir=os.environ.get("KTRACE_DIR", "/root/problem/trace_out"))
    res = run_bass_kernel_spmd(nc, in_maps, core_ids=list(range(8)), **kw)
    LAST_RESULT["res"] = res
    parts = [res.results[c]["out"] for c in range(8)]          # each [1024, VS], t-major rows
    full = np.concatenate(parts, axis=1)[:, :V].astype(np.float32)
    return np.ascontiguousarray(full.reshape(T, B, V).transpose(1, 0, 2))



# revision 67
# speedup vs baseline: 1.6508x; 1.0221x over previous
"""Trainium2 Bass kernel for nn_AgnisV5: 2-layer GPT encoder + gated
hierarchical recurrence + lm_head, SPMD over 8 NeuronCores.

Strategy: encoder + recurrence replicated on all cores (no collectives);
lm_head vocab-sharded 8 ways. Forward-pass simplification: stop_gradient is
identity, so the "settled" hierarchy path equals the differentiable shadow
path and blend == core_sh.

Layouts: feature-major activations [128p, K, N] (feature f = k*128 + p).
Encoder token columns are b-major (n = b*64 + t); recurrence/H/lm_head use
t-major (n = t*16 + b) via strided views of ctx.
"""
import numpy as np
import ml_dtypes
from contextlib import ExitStack

import concourse.bass as bass
import concourse.mybir as mybir
import concourse.tile as tile
from concourse import bacc
from concourse.bass_utils import run_bass_kernel_spmd

AF = mybir.ActivationFunctionType
OP = mybir.AluOpType
BF = mybir.dt.float16
F32 = mybir.dt.float32

D, HID, FFN, NH, NL, T, V, B = 768, 3072, 2048, 8, 2, 64, 50257, 16
HD = 96
KD = D // 128          # 6
MH = HID // 128        # 24
MF = FFN // 128        # 16
NTOK = B * T           # 1024
MT = NTOK // 128       # 8
VP = 53248             # padded vocab = 8*6656
VS = VP // 8           # 6656 per core
NCH = VS // 512        # 13
ALPHA = 0.4
QK_SCALE = 1.0 / np.sqrt(96.0)

bfnp = np.float16


_FLAGS = {"ln_triv": False, "bv0": False, "b2z": False}


def bc(ap, reps, pos):
    """Insert a stride-0 (broadcast) free dim at position pos (1-based over free dims)."""
    newap = list(ap.ap)
    newap.insert(pos, [0, reps])
    return bass.AP(tensor=ap.tensor, offset=ap.offset, ap=newap)


def build_nc():
    LN_TRIV = _FLAGS["ln_triv"]
    BV0 = _FLAGS["bv0"]
    B2Z = _FLAGS["b2z"]
    nc = bacc.Bacc(None, target_bir_lowering=False)

    # ---- DRAM I/O ----
    x0T = nc.dram_tensor("x0T", [D, NTOK], BF, kind="ExternalInput")
    enc_in = {}
    for l in range(NL):
        enc_in[f"wqk{l}"] = nc.dram_tensor(f"wqk{l}", [D, 2048], BF, kind="ExternalInput")
        enc_in[f"wv{l}"] = nc.dram_tensor(f"wv{l}", [D, D], BF, kind="ExternalInput")
        enc_in[f"wao{l}"] = nc.dram_tensor(f"wao{l}", [1024, D], BF, kind="ExternalInput")
        enc_in[f"ff1T{l}"] = nc.dram_tensor(f"ff1T{l}", [D, FFN], BF, kind="ExternalInput")
        enc_in[f"ff2T{l}"] = nc.dram_tensor(f"ff2T{l}", [FFN, D], BF, kind="ExternalInput")
        enc_in[f"bqk{l}"] = nc.dram_tensor(f"bqk{l}", [128, 16], F32, kind="ExternalInput")
        enc_in[f"bv{l}"] = nc.dram_tensor(f"bv{l}", [1, D], BF, kind="ExternalInput")
        enc_in[f"bao{l}"] = nc.dram_tensor(f"bao{l}", [128, KD], F32, kind="ExternalInput")
        enc_in[f"bff1{l}"] = nc.dram_tensor(f"bff1{l}", [128, MF], F32, kind="ExternalInput")
        enc_in[f"bff2{l}"] = nc.dram_tensor(f"bff2{l}", [128, KD], F32, kind="ExternalInput")
        enc_in[f"g1{l}"] = nc.dram_tensor(f"g1{l}", [128, KD], F32, kind="ExternalInput")
        enc_in[f"bb1{l}"] = nc.dram_tensor(f"bb1{l}", [128, KD], F32, kind="ExternalInput")
        enc_in[f"g2{l}"] = nc.dram_tensor(f"g2{l}", [128, KD], F32, kind="ExternalInput")
        enc_in[f"bb2{l}"] = nc.dram_tensor(f"bb2{l}", [128, KD], F32, kind="ExternalInput")
    encg = nc.dram_tensor("encg", [128, KD], F32, kind="ExternalInput")
    encb = nc.dram_tensor("encb", [128, KD], F32, kind="ExternalInput")
    maskT = nc.dram_tensor("maskT", [64, 64], BF, kind="ExternalInput")
    V0d = nc.dram_tensor("V0d", [D, HID], BF, kind="ExternalInput")
    V1d = nc.dram_tensor("V1d", [HID, D], BF, kind="ExternalInput")
    Whud = nc.dram_tensor("Whud", [D, HID], BF, kind="ExternalInput")
    Wtfd = nc.dram_tensor("Wtfd", [D, D], BF, kind="ExternalInput")
    cw1d = nc.dram_tensor("cw1d", [D, D], BF, kind="ExternalInput")
    cw2d = nc.dram_tensor("cw2d", [D, D], BF, kind="ExternalInput")
    gw1d = nc.dram_tensor("gw1d", [D, D], BF, kind="ExternalInput")
    gw2d = nc.dram_tensor("gw2d", [D, D], BF, kind="ExternalInput")
    b0d = nc.dram_tensor("b0d", [128, MH], F32, kind="ExternalInput")
    b1d = nc.dram_tensor("b1d", [128, KD], F32, kind="ExternalInput")
    cb1d = nc.dram_tensor("cb1d", [128, KD], F32, kind="ExternalInput")
    cb2d = nc.dram_tensor("cb2d", [128, KD], F32, kind="ExternalInput")
    gbd = nc.dram_tensor("gbd", [128, KD], F32, kind="ExternalInput")
    outgd = nc.dram_tensor("outgd", [128, KD], F32, kind="ExternalInput")
    outbd = nc.dram_tensor("outbd", [128, KD], F32, kind="ExternalInput")
    lmTd = nc.dram_tensor("lmTd", [D, VS], BF, kind="ExternalInput")
    eyed = nc.dram_tensor("eyed", [128, 128], BF, kind="ExternalInput")
    outd = nc.dram_tensor("out", [NTOK, VS], BF, kind="ExternalOutput")
    import os as _os0
    KDEBUG = bool(_os0.environ.get("KDEBUG"))
    if KDEBUG:
        dbg_ctx = nc.dram_tensor("dbg_ctx", [128, KD * NTOK], BF, kind="ExternalOutput")
        dbg_uc = nc.dram_tensor("dbg_uc", [128, MH * 128], BF, kind="ExternalOutput")
        dbg_h = nc.dram_tensor("dbg_h", [128, KD * 128], BF, kind="ExternalOutput")
        dbg_st = {}
        for nm, width, dt_ in [("hs", MH * B, BF), ("cpf", KD * B, F32),
                               ("core", KD * B, BF), ("gm", KD * B, BF),
                               ("cf", KD * B, F32), ("tg", KD * B, F32),
                               ("hp", KD * B, F32), ("ln", KD * B, F32)]:
            dbg_st[nm] = nc.dram_tensor(f"dbg_{nm}", [128, width], dt_,
                                        kind="ExternalOutput")

    def r3(h, p=128):
        return h[:, :].rearrange("(k p) n -> p k n", p=p)

    with ExitStack() as ctx:
        tc = ctx.enter_context(tile.TileContext(nc))
        cross = ctx.enter_context(tc.tile_pool(name="cross", bufs=1))

        # constants (cross-scope)
        ones_col = cross.tile([128, 1], BF, tag="ones_col")
        nc.vector.memset(ones_col, 1.0)
        ones64 = cross.tile([64, 1], BF, tag="ones64")
        nc.vector.memset(ones64, 1.0)
        ones_row_f = cross.tile([1, 128], F32, tag="ones_row_f")
        nc.vector.memset(ones_row_f, 1.0)
        ones_r64f = cross.tile([1, 64], F32, tag="ones_r64f")
        nc.vector.memset(ones_r64f, 1.0)
        mask_sb = cross.tile([64, 64], BF, tag="mask")
        nc.sync.dma_start(out=mask_sb, in_=maskT[:, :])
        eye = cross.tile([128, 128], BF, tag="eye")
        nc.sync.dma_start(out=eye, in_=eyed[:, :])
        eps5 = cross.tile([128, 1], F32, tag="eps5")
        nc.vector.memset(eps5, 1e-5)
        eps24 = cross.tile([128, 1], F32, tag="eps24")
        nc.vector.memset(eps24, 1e-24)
        ctxF = cross.tile([128, KD, NTOK], BF, tag="ctxF")

        # ================= ENCODER SCOPE =================
        with tc.tile_pool(name="wenc", bufs=1) as wenc, \
             tc.tile_pool(name="bige", bufs=1) as bige, \
             tc.tile_pool(name="sme", bufs=1) as sme, \
             tc.tile_pool(name="pmm", bufs=6, space="PSUM") as pmm, \
             tc.tile_pool(name="pbc", bufs=1, space="PSUM") as pbc:

            def ln_fm(x_sb, g_ap, b_ap, out, l2_out=None, triv=False):
                """x_sb [128, KD, 1024] bf16 -> LN -> out; optional l2 -> l2_out.
                triv: gamma==1 and beta==0 (runtime-specialized)."""
                for j in range(2):
                    jj = slice(j * 512, (j + 1) * 512)
                    s12 = pbc.tile([128, 2, 512], F32, tag="pabc", bufs=1,
                                   name="s12")[:1]
                    for k in range(KD):
                        sq = bige.tile([128, 512], BF, tag="sqc", bufs=2)
                        nc.vector.tensor_mul(sq, x_sb[:, k, jj], x_sb[:, k, jj])
                        nc.tensor.matmul(s12[:, 0, :], ones_col, x_sb[:, k, jj],
                                         start=(k == 0), stop=(k == KD - 1))
                        nc.tensor.matmul(s12[:, 1, :], ones_col, sq,
                                         start=(k == 0), stop=(k == KD - 1))
                    m = sme.tile([1, 512], F32, tag="mtag", bufs=2)
                    nc.vector.tensor_scalar_mul(m, s12[:, 0, :], 1.0 / D)
                    msq = sme.tile([1, 512], F32, tag="scr512", bufs=2)
                    nc.vector.tensor_mul(msq, m, m)
                    var = sme.tile([1, 512], F32, tag="scr512", bufs=2)
                    nc.vector.scalar_tensor_tensor(var, s12[:, 1, :], 1.0 / D,
                                                   msq, OP.mult, OP.subtract)
                    sd = sme.tile([1, 512], F32, tag="scr512", bufs=2)
                    nc.scalar.activation(sd, var, AF.Sqrt, bias=eps5[:1, :])
                    ac = sme.tile([1, 2, 512], F32, tag="acf")
                    nc.vector.reciprocal(ac[:, 0, :], sd)
                    nc.vector.scalar_tensor_tensor(ac[:, 1, :], m, -1.0,
                                                   ac[:, 0, :], OP.mult, OP.mult)
                    pac = pbc.tile([128, 2, 512], F32, tag="pabc", bufs=1)
                    for a in range(2):
                        nc.tensor.matmul(pac[:, a, :], ones_row_f, ac[:, a, :],
                                         start=True, stop=True)
                    pab = bige.tile([128, 2, 512], BF, tag="pab16", bufs=2)
                    nc.scalar.activation(pab, pac, AF.Copy)
                    for k in range(KD):
                        if triv:
                            t1 = bige.tile([128, 512], BF, tag="t1", bufs=2)
                            nc.vector.tensor_mul(t1, x_sb[:, k, jj], pab[:, 0, :])
                            nc.vector.tensor_add(out[:, k, jj], t1, pab[:, 1, :])
                        else:
                            t1 = bige.tile([128, 512], F32, tag="t1f", bufs=2)
                            nc.vector.tensor_mul(t1, x_sb[:, k, jj], pab[:, 0, :])
                            nc.vector.tensor_add(t1, t1, pab[:, 1, :])
                            nc.vector.tensor_scalar(out[:, k, jj], t1,
                                                    g_ap[:, k:k + 1],
                                                    b_ap[:, k:k + 1],
                                                    OP.mult, OP.add)
                if l2_out is None:
                    return
                for j in range(2):
                    jj = slice(j * 512, (j + 1) * 512)
                    s1t = pbc.tile([128, 2, 512], F32, tag="pabc", bufs=1,
                                   name="s1t")[:1]
                    s1 = s1t[:, 0, :]
                    for k in range(KD):
                        sq = bige.tile([128, 512], BF, tag="sqc", bufs=2)
                        nc.vector.tensor_mul(sq, out[:, k, jj], out[:, k, jj])
                        nc.tensor.matmul(s1, ones_col, sq,
                                         start=(k == 0), stop=(k == KD - 1))
                    sd = sme.tile([1, 512], F32, tag="scr512", bufs=2)
                    nc.scalar.activation(sd, s1, AF.Sqrt, bias=eps24[:1, :])
                    rr = sme.tile([1, 512], F32, tag="scr512", bufs=2)
                    nc.vector.reciprocal(rr, sd)
                    pat = pbc.tile([128, 2, 512], F32, tag="pabc", bufs=1)
                    pa = pat[:, 0, :]
                    nc.tensor.matmul(pa, ones_row_f, rr, start=True, stop=True)
                    pa16 = bige.tile([128, 512], BF, tag="pa16", bufs=2)
                    nc.scalar.activation(pa16, pa, AF.Copy)
                    for k in range(KD):
                        nc.vector.tensor_mul(l2_out[:, k, jj], out[:, k, jj], pa16)

            X = bige.tile([128, KD, NTOK], BF, tag="X")
            nc.sync.dma_start(out=X, in_=r3(x0T))
            gA = sme.tile([128, NL, KD], F32, tag="gA")
            bA = sme.tile([128, NL, KD], F32, tag="bA")
            gB = sme.tile([128, NL, KD], F32, tag="gB")
            bB = sme.tile([128, NL, KD], F32, tag="bB")
            for l in range(NL):
                nc.sync.dma_start(out=gA[:, l, :], in_=enc_in[f"g1{l}"][:, :])
                nc.sync.dma_start(out=bA[:, l, :], in_=enc_in[f"bb1{l}"][:, :])
                nc.sync.dma_start(out=gB[:, l, :], in_=enc_in[f"g2{l}"][:, :])
                nc.sync.dma_start(out=bB[:, l, :], in_=enc_in[f"bb2{l}"][:, :])

            for l in range(NL):
                XN = bige.tile([128, KD, NTOK], BF, tag="XN")
                ln_fm(X, gA[:, l, :], bA[:, l, :], XN, triv=LN_TRIV)
                wqk = wenc.tile([128, KD, 2048], BF, tag="wbig")
                nc.sync.dma_start(out=wqk, in_=r3(enc_in[f"wqk{l}"]))
                bqk = sme.tile([128, 16], F32, tag="bqk")
                nc.sync.dma_start(out=bqk, in_=enc_in[f"bqk{l}"][:, :])
                # v projection (token-major)
                wv = wenc.tile([128, KD, D], BF, tag="wsmall")
                nc.sync.dma_start(out=wv, in_=r3(enc_in[f"wv{l}"]))
                if not BV0:
                    ones_row = sme.tile([1, 128], BF, tag="ones_row")
                    nc.vector.memset(ones_row, 1.0)
                    bvr = sme.tile([1, D], BF, tag="bvr")
                    nc.sync.dma_start(out=bvr, in_=enc_in[f"bv{l}"][:, :])
                    bvs = bige.tile([128, D], BF, tag="bvs")
                    for j in range(2):
                        pb = pbc.tile([128, 2, 512], F32, tag="pabc", bufs=1)
                        nc.tensor.matmul(pb[:, 0, :384], ones_row,
                                         bvr[:, j * 384:(j + 1) * 384],
                                         start=True, stop=True)
                        nc.vector.tensor_copy(bvs[:, j * 384:(j + 1) * 384],
                                              pb[:, 0, :384])
                vS = bige.tile([64, B, D], BF, tag="vS")
                for b in range(B):
                    for j in range(2):
                        jj = slice(j * 384, (j + 1) * 384)
                        ps = pmm.tile([128, 512], F32, tag="pmm", name="psv")
                        for k in range(KD):
                            nc.tensor.matmul(ps[:64, :384],
                                             XN[:, k, b * 64:(b + 1) * 64],
                                             wv[:, k, jj],
                                             start=(k == 0), stop=(k == KD - 1))
                        if not BV0:
                            nc.vector.tensor_add(vS[:, b, jj], ps[:64, :384],
                                                 bvs[:64, jj])
                        elif (2 * b + j) % 2 == 0:
                            nc.scalar.activation(vS[:, b, jj], ps[:64, :384],
                                                 AF.Copy)
                        else:
                            nc.vector.tensor_copy(vS[:, b, jj], ps[:64, :384])
                # attention per head; additive mask injected into score PSUM
                ofS = bige.tile([128, NH, NTOK], BF, tag="ofS")
                nc.vector.memset(ofS[96:128, :, :], 0.0)
                for h in range(NH):
                    qh = bige.tile([128, NTOK], BF, tag="qh", bufs=2)
                    kh = bige.tile([128, NTOK], BF, tag="kh", bufs=2)
                    for j in range(2):
                        jj = slice(j * 512, (j + 1) * 512)
                        ph = pmm.tile([128, 512], F32, tag="pmm", name="phq")
                        for k in range(KD):
                            nc.tensor.matmul(ph,
                                             wqk[:, k, h * 128:(h + 1) * 128],
                                             XN[:, k, jj],
                                             start=(k == 0), stop=(k == KD - 1))
                        if (h + j) % 2 == 0:
                            nc.scalar.activation(qh[:, jj], ph, AF.Identity,
                                                 bias=bqk[:, h:h + 1])
                        else:
                            nc.vector.tensor_scalar_add(qh[:, jj], ph,
                                                        bqk[:, h:h + 1])
                        ph2 = pmm.tile([128, 512], F32, tag="pmm", name="phk")
                        for k in range(KD):
                            nc.tensor.matmul(ph2,
                                             wqk[:, k, 1024 + h * 128:1024 + (h + 1) * 128],
                                             XN[:, k, jj],
                                             start=(k == 0), stop=(k == KD - 1))
                        if (h + j) % 2 == 0:
                            nc.vector.tensor_scalar_add(kh[:, jj], ph2,
                                                        bqk[:, 8 + h:9 + h])
                        else:
                            nc.scalar.activation(kh[:, jj], ph2, AF.Identity,
                                                 bias=bqk[:, 8 + h:9 + h])
                    eT = bige.tile([64, NTOK], BF, tag="eT", bufs=3)
                    for j in range(2):
                        jj = slice(j * 512, (j + 1) * 512)
                        sT = pmm.tile([128, 512], F32, tag="pmm", name="sT")
                        for b in range(8 * j, 8 * j + 8):
                            c = slice((b % 8) * 64, (b % 8) * 64 + 64)
                            nc.tensor.matmul(sT[:64, c],
                                             kh[:, b * 64:(b + 1) * 64],
                                             qh[:, b * 64:(b + 1) * 64],
                                             start=True, stop=False)
                            nc.tensor.matmul(sT[:64, c],
                                             eye[:64, :64], mask_sb,
                                             start=False, stop=True)
                        nc.scalar.activation(eT[:, jj], sT[:64, :], AF.Exp,
                                             scale=QK_SCALE)
                    ssum = pbc.tile([128, 2, 512], F32, tag="pabc", bufs=1,
                                    name="ssum")[:1]
                    for a in range(2):
                        nc.tensor.matmul(ssum[:, a, :], ones64,
                                         eT[:, a * 512:(a + 1) * 512],
                                         start=True, stop=True)
                    rs2 = sme.tile([1, 2, 512], F32, tag="acf")
                    rs = rs2[:, :, :].rearrange("p a c -> p (a c)")
                    nc.vector.reciprocal(rs, ssum[:, :, :].rearrange("p a c -> p (a c)"))
                    eN = bige.tile([64, NTOK], BF, tag="eN", bufs=3)
                    for a in range(2):
                        aa = slice(a * 512, (a + 1) * 512)
                        rb64 = pmm.tile([128, 512], F32, tag="pmm", name="rb64")
                        nc.tensor.matmul(rb64[:64, :], ones_r64f, rs[:, aa],
                                         start=True, stop=True)
                        nc.vector.tensor_mul(eN[:, aa], eT[:, aa], rb64[:64, :])
                    for j in range(2):
                        jj = slice(j * 512, (j + 1) * 512)
                        oh = pmm.tile([128, 512], F32, tag="pmm", name="oh")
                        for b in range(8 * j, 8 * j + 8):
                            c = slice((b % 8) * 64, (b % 8) * 64 + 64)
                            nc.tensor.matmul(oh[:HD, c],
                                             vS[:, b, h * HD:(h + 1) * HD],
                                             eN[:, b * 64:(b + 1) * 64],
                                             start=True, stop=True)
                        nc.scalar.activation(ofS[:HD, h, jj], oh[:HD, :], AF.Copy)
                # attn out + residual (in place on X)
                wao = wenc.tile([128, NH, D], BF, tag="wsmall")
                nc.sync.dma_start(out=wao,
                                  in_=enc_in[f"wao{l}"][:, :].rearrange("(k p) n -> p k n", p=128))
                bao = sme.tile([128, KD], F32, tag="bao")
                nc.sync.dma_start(out=bao, in_=enc_in[f"bao{l}"][:, :])
                for mo in range(KD):
                    for j in range(2):
                        jj = slice(j * 512, (j + 1) * 512)
                        ps = pmm.tile([128, 512], F32, tag="pmm", name="pswo")
                        for kv in range(NH):
                            nc.tensor.matmul(ps,
                                             wao[:, kv, mo * 128:(mo + 1) * 128],
                                             ofS[:, kv, jj],
                                             start=(kv == 0), stop=(kv == NH - 1))
                        if B2Z:
                            nc.vector.tensor_add(X[:, mo, jj], X[:, mo, jj], ps)
                        else:
                            t2 = bige.tile([128, 512], F32, tag="t2", bufs=2)
                            nc.vector.tensor_scalar_add(t2, ps, bao[:, mo:mo + 1])
                            nc.vector.tensor_add(X[:, mo, jj], X[:, mo, jj], t2)
                # ffn
                XN2 = bige.tile([128, KD, NTOK], BF, tag="XN")
                ln_fm(X, gB[:, l, :], bB[:, l, :], XN2, triv=LN_TRIV)
                w1 = wenc.tile([128, KD, FFN], BF, tag="wbig")
                nc.sync.dma_start(out=w1, in_=r3(enc_in[f"ff1T{l}"]))
                bf1 = sme.tile([128, MF], F32, tag="bf1")
                nc.sync.dma_start(out=bf1, in_=enc_in[f"bff1{l}"][:, :])
                G = bige.tile([128, MF, NTOK], BF, tag="G")
                for mf in range(MF):
                    for j in range(2):
                        jj = slice(j * 512, (j + 1) * 512)
                        ps = pmm.tile([128, 512], F32, tag="pmm", name="psf1")
                        for k in range(KD):
                            nc.tensor.matmul(ps,
                                             w1[:, k, mf * 128:(mf + 1) * 128],
                                             XN2[:, k, jj],
                                             start=(k == 0), stop=(k == KD - 1))
                        nc.scalar.activation(G[:, mf, jj], ps, AF.Gelu,
                                             bias=bf1[:, mf:mf + 1])
                w2 = wenc.tile([128, MF, D], BF, tag="wbig")
                nc.sync.dma_start(out=w2, in_=r3(enc_in[f"ff2T{l}"]))
                bf2 = sme.tile([128, KD], F32, tag="bao")
                nc.sync.dma_start(out=bf2, in_=enc_in[f"bff2{l}"][:, :])
                for mo in range(KD):
                    for j in range(2):
                        jj = slice(j * 512, (j + 1) * 512)
                        ps = pmm.tile([128, 512], F32, tag="pmm", name="psf2")
                        for kf in range(MF):
                            nc.tensor.matmul(ps,
                                             w2[:, kf, mo * 128:(mo + 1) * 128],
                                             G[:, kf, jj],
                                             start=(kf == 0), stop=(kf == MF - 1))
                        if B2Z:
                            nc.vector.tensor_add(X[:, mo, jj], X[:, mo, jj], ps)
                        else:
                            t2 = bige.tile([128, 512], F32, tag="t2", bufs=2)
                            nc.vector.tensor_scalar_add(t2, ps, bf2[:, mo:mo + 1])
                            nc.vector.tensor_add(X[:, mo, jj], X[:, mo, jj], t2)

            # final norm + l2 -> ctxF (cross pool)
            eg = sme.tile([128, KD], F32, tag="eg")
            eb = sme.tile([128, KD], F32, tag="eb")
            nc.sync.dma_start(out=eg, in_=encg[:, :])
            nc.sync.dma_start(out=eb, in_=encb[:, :])
            pre = bige.tile([128, KD, NTOK], BF, tag="XN")
            ln_fm(X, eg, eb, pre, l2_out=ctxF, triv=LN_TRIV)


        # ================= RECURRENCE SCOPE =================
        ctx_r = ctxF[:, :, :].rearrange("p k (b t) -> p t k b", b=B)
        with tc.tile_pool(name="wrec", bufs=1) as wrec, \
             tc.tile_pool(name="smr", bufs=2) as smr, \
             tc.tile_pool(name="prc", bufs=1, space="PSUM") as prc, \
             tc.tile_pool(name="pr6", bufs=1, space="PSUM") as pr6, \
             tc.tile_pool(name="prs", bufs=1, space="PSUM") as prs, \
             tc.tile_pool(name="pou", bufs=1, space="PSUM") as pou:

            Whu = wrec.tile([128, KD, HID], BF, tag="Whu")
            nc.sync.dma_start(out=Whu, in_=r3(Whud))
            V0w = wrec.tile([128, KD, HID], BF, tag="V0w")
            nc.sync.dma_start(out=V0w, in_=r3(V0d))
            gw1w = wrec.tile([128, KD, D], BF, tag="gw1w")
            nc.sync.dma_start(out=gw1w, in_=r3(gw1d))
            V1w = wrec.tile([128, MH, D], BF, tag="V1w")
            nc.sync.dma_start(out=V1w, in_=r3(V1d))
            Wtf = wrec.tile([128, KD, D], BF, tag="Wtf")
            nc.sync.dma_start(out=Wtf, in_=r3(Wtfd))
            cw1 = wrec.tile([128, KD, D], BF, tag="cw1")
            nc.sync.dma_start(out=cw1, in_=r3(cw1d))
            cw2 = wrec.tile([128, KD, D], BF, tag="cw2")
            nc.sync.dma_start(out=cw2, in_=r3(cw2d))
            gw2 = wrec.tile([128, KD, D], BF, tag="gw2")
            nc.sync.dma_start(out=gw2, in_=r3(gw2d))
            b1S = smr.tile([128, KD], F32, tag="b1S", bufs=1)
            nc.sync.dma_start(out=b1S, in_=b1d[:, :])
            cb1S = smr.tile([128, KD], F32, tag="cb1S", bufs=1)
            nc.sync.dma_start(out=cb1S, in_=cb1d[:, :])
            cb2S = smr.tile([128, KD], F32, tag="cb2S", bufs=1)
            nc.sync.dma_start(out=cb2S, in_=cb2d[:, :])
            # bf16 broadcast copies (B cols) for PSUM bias injection
            b0S = smr.tile([128, MH], F32, tag="b0S", bufs=1)
            nc.sync.dma_start(out=b0S, in_=b0d[:, :])
            b0b = smr.tile([128, MH, B], BF, tag="b0b", bufs=1)
            nc.vector.tensor_copy(b0b, bc(b0S[:, :], B, 2))
            gbS = smr.tile([128, KD], F32, tag="gbS", bufs=1)
            nc.sync.dma_start(out=gbS, in_=gbd[:, :])
            gbb = smr.tile([128, KD, B], BF, tag="gbb", bufs=1)
            nc.vector.tensor_copy(gbb, bc(gbS[:, :], B, 2))
            b1b = smr.tile([128, KD, B], BF, tag="b1b", bufs=1)
            nc.vector.tensor_copy(b1b, bc(b1S[:, :], B, 2))
            cb1b = smr.tile([128, KD, B], BF, tag="cb1b", bufs=1)
            nc.vector.tensor_copy(cb1b, bc(cb1S[:, :], B, 2))
            cb2b = smr.tile([128, KD, B], BF, tag="cb2b", bufs=1)
            nc.vector.tensor_copy(cb2b, bc(cb2S[:, :], B, 2))
            outgS = smr.tile([128, KD], F32, tag="outgS", bufs=1)
            nc.sync.dma_start(out=outgS, in_=outgd[:, :])
            outbS = smr.tile([128, KD], F32, tag="outbS", bufs=1)
            nc.sync.dma_start(out=outbS, in_=outbd[:, :])
            Hc = [wrec.tile([128, KD, 128], BF, tag=f"H{mt}", name=f"H{mt}")
                  for mt in range(MT)]

            import os as _os
            T_RUN = int(_os.environ.get("T_TRUNC", T))
            if T_RUN < T:
                for Hcm in Hc:
                    nc.vector.memset(Hcm, 0.0)

            NBLK = (T_RUN + 7) // 8
            if KDEBUG:
                nc.sync.dma_start(out=dbg_ctx[:, :],
                                  in_=ctxF[:, :, :].rearrange("p k n -> p (k n)"))

            # lm chunk emission: during step t of block w, process chunks of
            # block w-1 (Hc[w-1] is complete); 13 chunks spread over 8 steps.
            LM_SPLIT = [0, 2, 4, 6, 8, 10, 11, 12, 13]

            def lm_chunks(mt, j0, j1):
                for j in range(j0, j1):
                    lw = wrec.tile([128, KD, 512], BF, tag="lmw", bufs=2)
                    nc.sync.dma_start(
                        out=lw,
                        in_=lmTd[:, j * 512:(j + 1) * 512].rearrange("(k p) n -> p k n",
                                                                     p=128))
                    ps = pou.tile([128, 512], F32, tag="plm", bufs=1)
                    for k in range(KD):
                        nc.tensor.matmul(ps, Hc[mt][:, k, :], lw[:, k, :],
                                         start=(k == 0), stop=(k == KD - 1))
                    ot = smr.tile([128, 512], BF, tag="otile", bufs=3)
                    if j % 2 == 0:
                        nc.scalar.activation(ot, ps, AF.Copy)
                    else:
                        nc.vector.tensor_copy(ot, ps)
                    nc.sync.dma_start(out=outd[mt * 128:(mt + 1) * 128,
                                               j * 512:(j + 1) * 512], in_=ot)

            u_tiles = {}
            gp_tiles = {}

            def emit_u_prefix(tt, mlo=0, mhi=MH):
                # b0 + V0^T ctx accumulation prefix for step tt (h-independent),
                # emitted in chunks interleaved with Pool-LN calls so the PE
                # runs it during those idle windows. One psum bank: single
                # start (zeroes whole 2KB region), single stop at group end.
                if mlo == 0:
                    u_tiles[tt] = prc.tile([128, 32, B], F32, tag="pu", bufs=2,
                                           name=f"u{tt}")[:, :MH, :]
                un = u_tiles[tt]
                cx = ctx_r[:, tt]
                for m in range(mlo, mhi):
                    nc.tensor.matmul(un[:, m, :], eye, b0b[:, m, :],
                                     start=(m == 0), stop=False)
                    for k in range(KD):
                        nc.tensor.matmul(un[:, m, :], V0w[:, k, m * 128:(m + 1) * 128],
                                         cx[:, k, :], start=False,
                                         stop=(tt == 0 and m == MH - 1 and k == KD - 1))

            def emit_gp_prefix(tt, mlo=0, mhi=KD):
                if mlo == 0:
                    gp_tiles[tt] = pr6.tile([128, 8, B], F32, tag="pgp", bufs=2,
                                            name=f"gp{tt}")[:, :KD, :]
                gpn = gp_tiles[tt]
                cx = ctx_r[:, tt]
                for m in range(mlo, mhi):
                    nc.tensor.matmul(gpn[:, m, :], eye, gbb[:, m, :],
                                     start=(m == 0), stop=False)
                    for k in range(KD):
                        nc.tensor.matmul(gpn[:, m, :],
                                         gw1w[:, k, m * 128:(m + 1) * 128],
                                         cx[:, k, :], start=False, stop=False)

            emit_u_prefix(0)
            emit_gp_prefix(0)
            for t in range(T_RUN):
                ctx_t = ctx_r[:, t]
                mtb, off = t // 8, (t % 8) * B
                if t > 0:
                    hprev = Hc[(t - 1) // 8][:, :, ((t - 1) % 8) * B:((t - 1) % 8) * B + B]
                u = u_tiles.pop(t)
                if t > 0:
                    for half in range(2):
                        cc = slice(8 * half, 8 * half + 8)
                        for m in range(MH):
                            for k in range(KD):
                                nc.tensor.matmul(
                                    u[:, m, cc], Whu[:, k, m * 128:(m + 1) * 128],
                                    hprev[:, k, cc], start=False,
                                    stop=(half == 1 and m == MH - 1 and k == KD - 1))
                hsS = smr.tile([128, MH, B], BF, tag="hsS")
                nc.scalar.activation(hsS, u, AF.Gelu)
                # tf branch (off critical path): w = tf - ctx
                w_t = smr.tile([128, KD, B], F32, tag="w_t")
                if t > 0:
                    tfp = prc.tile([128, 8, B], F32, tag="ptf", name="tfp")[:, :KD, :]
                    for m in range(KD):
                        for k in range(KD):
                            nc.tensor.matmul(tfp[:, m, :], Wtf[:, k, m * 128:(m + 1) * 128],
                                             hprev[:, k, :], start=(k == 0),
                                             stop=(k == KD - 1))
                    nc.vector.tensor_sub(w_t, tfp, ctx_t)
                else:
                    nc.vector.tensor_scalar_mul(w_t, ctx_t, -1.0)
                # V1 split into column halves in separate psum banks so the
                # second half runs under the first half's gelu + rmsnorm calls
                cpf = smr.tile([128, B, KD], F32, tag="cpf")
                for half in range(2):
                    cc = slice(8 * half, 8 * half + 8)
                    cprh = pr6.tile([128, 8, B], F32, tag="p6", bufs=2,
                                    name=f"cpr{half}")[:, :KD, :]
                    for m in range(KD):
                        nc.tensor.matmul(cprh[:, m, cc], eye, b1b[:, m, cc],
                                         start=(m == 0), stop=False)
                    for m in range(KD):
                        for k in range(MH):
                            nc.tensor.matmul(cprh[:, m, cc],
                                             V1w[:, k, m * 128:(m + 1) * 128],
                                             hsS[:, k, cc], start=False,
                                             stop=(m == KD - 1 and k == MH - 1))
                    nc.scalar.activation(
                        cpf[:, cc, :].rearrange("p b k -> p k b"),
                        cprh[:, :, cc], AF.Gelu)
                # l2norm via gpsimd rmsnorm (no act-table); 1/sqrt(D) folded
                # into cw1 host-side.
                corePool = smr.tile([128, B, KD], F32, tag="corePool")
                coreS = smr.tile([128, KD, B], BF, tag="coreS")
                gm = pr6.tile([128, 8, B], F32, tag="p6", bufs=2, name="gm")[:, :KD, :]
                for b in range(B):
                    nc.gpsimd.layernorm(corePool[:, b, :], cpf[:, b, :],
                                        subtract_mean=False, eps=1e-10)
                    if b % 4 == 3 and t + 1 < T_RUN:
                        emit_u_prefix(t + 1, 6 * (b // 4), 6 * (b // 4 + 1))
                    if b % 8 == 7:
                        # half-batch: copy + start cw1 while the other half's
                        # rmsnorm calls still run on Pool
                        half = b // 8
                        cc = slice(8 * half, 8 * half + 8)
                        nc.vector.tensor_copy(
                            coreS[:, :, cc],
                            corePool[:, cc, :].rearrange("p b k -> p k b"))
                        if half == 0:
                            for m in range(KD):
                                nc.tensor.matmul(gm[:, m, :], eye, cb1b[:, m, :],
                                                 start=(m == 0), stop=False)
                        for m in range(KD):
                            for k in range(KD):
                                nc.tensor.matmul(
                                    gm[:, m, cc],
                                    cw1[:, k, m * 128:(m + 1) * 128],
                                    coreS[:, k, cc], start=False,
                                    stop=(half == 1 and m == KD - 1 and k == KD - 1))
                gmS = smr.tile([128, KD, B], BF, tag="gmS")
                nc.scalar.activation(gmS, gm, AF.Gelu)
                cfp = pr6.tile([128, 8, B], F32, tag="p6", bufs=2, name="cfp")[:, :KD, :]
                for m in range(KD):
                    for k in range(KD):
                        nc.tensor.matmul(cfp[:, m, :], cw2[:, k, m * 128:(m + 1) * 128],
                                         gmS[:, k, :], start=(k == 0), stop=False)
                    nc.tensor.matmul(cfp[:, m, :], eye, cb2b[:, m, :],
                                     start=False, stop=True)
                cfF = smr.tile([128, KD, B], F32, tag="cfF")
                nc.scalar.activation(cfF, cfp, AF.Copy)
                # gate cf-branch folded through cp_w2: reads g1 (gmS) directly,
                # in parallel with the cfp/cfF branch
                gp = gp_tiles.pop(t)
                for m in range(KD):
                    for k in range(KD):
                        nc.tensor.matmul(gp[:, m, :], gw2[:, k, m * 128:(m + 1) * 128],
                                         gmS[:, k, :], start=False,
                                         stop=(m == KD - 1 and k == KD - 1))
                # gate via tanh (shares the gelu act table):
                #   sigmoid(x) = 0.5*(1 + tanh(x/2))
                #   hp = gate*(cf+tf-ctx) + ctx = 0.5*(tanh+1)*(cf+tf-ctx) + ctx
                a1 = smr.tile([128, KD, B], F32, tag="a1")
                nc.vector.tensor_add(a1, cfF, w_t)
                tG = smr.tile([128, KD, B], F32, tag="gateS")
                q1 = smr.tile([128, KD, B], F32, tag="q1")
                hp = smr.tile([128, B, KD], F32, tag="hp")
                for half in range(2):
                    cc = slice(8 * half, 8 * half + 8)
                    nc.scalar.activation(tG[:, :, cc], gp[:, :, cc], AF.Tanh,
                                         scale=0.5)
                    nc.vector.scalar_tensor_tensor(q1[:, :, cc], tG[:, :, cc], 1.0,
                                                   a1[:, :, cc], OP.add, OP.mult)
                    nc.vector.scalar_tensor_tensor(
                        hp[:, cc, :].rearrange("p b k -> p k b"),
                        q1[:, :, cc], 0.5, ctx_t[:, :, cc], OP.mult, OP.add)
                # LN + affine via gpsimd layernorm (no act-table), then clip
                # halves so next step's Whu starts under the second LN half.
                lnout = smr.tile([128, B, KD], F32, tag="lnout")
                for b in range(B):
                    nc.gpsimd.layernorm(lnout[:, b, :], hp[:, b, :],
                                        gamma_ap=outgS[:, :], beta_ap=outbS[:, :],
                                        subtract_mean=True, eps=1e-5)
                    if b % 8 == 7:
                        half = b // 8
                        cc = slice(8 * half, 8 * half + 8)
                        nc.vector.tensor_scalar(
                            Hc[t // 8][:, :, (t % 8) * B + 8 * half:
                                       (t % 8) * B + 8 * half + 8],
                            lnout[:, cc, :].rearrange("p b k -> p k b"),
                            5.0, -5.0, OP.min, OP.max)
                        if t + 1 < T_RUN:
                            emit_gp_prefix(t + 1, 3 * half, 3 * (half + 1))
                if KDEBUG and t == min(7, T_RUN - 1):
                    nc.sync.dma_start(out=dbg_h[:, :],
                                      in_=Hc[0][:, :, :].rearrange("p k c -> p (k c)"))
                if KDEBUG and t == int(_os.environ.get("KPROBE_T", 0)):
                    for nm, tile_ in [("hs", hsS), ("cpf", cpf), ("core", coreS),
                                      ("gm", gmS), ("cf", cfF), ("tg", tG),
                                      ("hp", hp), ("ln", lnout)]:
                        nc.sync.dma_start(
                            out=dbg_st[nm][:, :],
                            in_=tile_[:, :, :].rearrange("p a b -> p (a b)"))
                # interleave lm-head work for the previous (complete) block
                if t >= 8 and T_RUN == T:
                    lm_chunks(mtb - 1, LM_SPLIT[t % 8], LM_SPLIT[t % 8 + 1])

            # lm-head tail: last block (or all blocks on truncated debug runs)
            tail_blocks = range(NBLK - 1, NBLK) if T_RUN == T else range(NBLK)
            for mt in tail_blocks:
                lm_chunks(mt, 0, NCH)

    nc.finalize()
    return nc


_NC_CACHE = {}


def _get_nc():
    key = (_FLAGS["ln_triv"], _FLAGS["bv0"], _FLAGS["b2z"])
    if key not in _NC_CACHE:
        _NC_CACHE[key] = build_nc()
    return _NC_CACHE[key]


def _prep_inputs(inputs):
    f = lambda x: np.asarray(x, np.float32)
    tok = np.asarray(inputs["token_ids"]).astype(np.int64)
    emb, pos = f(inputs["emb"]), f(inputs["pos_emb"])
    x0 = emb[tok.reshape(-1)] + np.tile(pos[:T], (B, 1))
    com = {"x0T": x0.T.astype(bfnp)}
    aiw, aib = f(inputs["attn_in_w"]), f(inputs["attn_in_b"])
    aow, aob = f(inputs["attn_out_w"]), f(inputs["attn_out_b"])
    for l in range(NL):
        wqk = np.zeros((D, 2048), np.float32)
        bqk = np.zeros(2048, np.float32)
        wv = np.zeros((D, D), np.float32)
        bv = np.zeros(D, np.float32)
        wao = np.zeros((1024, D), np.float32)
        for h in range(NH):
            wqk[:, h * 128:h * 128 + HD] = aiw[l, h * HD:(h + 1) * HD, :].T
            wqk[:, 1024 + h * 128:1024 + h * 128 + HD] = aiw[l, D + h * HD:D + (h + 1) * HD, :].T
            bqk[h * 128:h * 128 + HD] = aib[l, h * HD:(h + 1) * HD]
            bqk[1024 + h * 128:1024 + h * 128 + HD] = aib[l, D + h * HD:D + (h + 1) * HD]
            wv[:, h * HD:(h + 1) * HD] = aiw[l, 2 * D + h * HD:2 * D + (h + 1) * HD, :].T
            bv[h * HD:(h + 1) * HD] = aib[l, 2 * D + h * HD:2 * D + (h + 1) * HD]
            wao[h * 128:h * 128 + HD, :] = aow[l][:, h * HD:(h + 1) * HD].T
        com[f"wqk{l}"] = wqk.astype(bfnp)
        com[f"bqk{l}"] = bqk.reshape(16, 128).T.copy()
        com[f"wv{l}"] = wv.astype(bfnp)
        com[f"bv{l}"] = bv.reshape(1, D).astype(bfnp)
        com[f"wao{l}"] = wao.astype(bfnp)
        com[f"bao{l}"] = aob[l].reshape(KD, 128).T.copy()
        com[f"ff1T{l}"] = f(inputs["ff_w1"])[l].T.astype(bfnp).copy()
        com[f"bff1{l}"] = f(inputs["ff_b1"])[l].reshape(MF, 128).T.copy()
        com[f"ff2T{l}"] = f(inputs["ff_w2"])[l].T.astype(bfnp).copy()
        com[f"bff2{l}"] = f(inputs["ff_b2"])[l].reshape(KD, 128).T.copy()
        com[f"g1{l}"] = f(inputs["n1_g"])[l].reshape(KD, 128).T.copy()
        com[f"bb1{l}"] = f(inputs["n1_b"])[l].reshape(KD, 128).T.copy()
        com[f"g2{l}"] = f(inputs["n2_g"])[l].reshape(KD, 128).T.copy()
        com[f"bb2{l}"] = f(inputs["n2_b"])[l].reshape(KD, 128).T.copy()
    com["encg"] = f(inputs["enc_norm_g"]).reshape(KD, 128).T.copy()
    com["encb"] = f(inputs["enc_norm_b"]).reshape(KD, 128).T.copy()
    tk, tq = np.meshgrid(np.arange(64), np.arange(64), indexing="ij")
    com["maskT"] = ((tk > tq) * -30000.0).astype(bfnp)
    com["eyed"] = np.eye(128, dtype=bfnp)
    com["V0d"] = f(inputs["V0"]).astype(bfnp)
    com["V1d"] = f(inputs["V1"]).astype(bfnp)
    R, tw = f(inputs["R"]), f(inputs["temp_w"])
    # folded recurrent weights: u += (alpha*R@V0)^T h ; tf = (alpha*R@tw^T)^T h
    com["Whud"] = (ALPHA * R @ f(inputs["V0"])).astype(bfnp)
    com["Wtfd"] = (ALPHA * R @ tw.T).astype(bfnp)
    # rmsnorm(x) = sqrt(D) * l2norm(x): fold the 1/sqrt(D) into cp_w1
    com["cw1d"] = (f(inputs["cp_w1"]).T / np.sqrt(D)).astype(bfnp).copy()
    com["cw2d"] = f(inputs["cp_w2"]).T.astype(bfnp).copy()
    gw = f(inputs["gate_w"])
    com["gw1d"] = gw[:, :D].T.astype(bfnp).copy()
    # gate cf-branch folded through cp_w2: gs_cf = Wgg^T g1 (+ gwB@cp_b2 -> gb)
    gwB = gw[:, D:]
    com["gw2d"] = np.ascontiguousarray((gwB @ f(inputs["cp_w2"])).T).astype(bfnp)
    com["gbd"] = (f(inputs["gate_b"]) + gwB @ f(inputs["cp_b2"])).reshape(KD, 128).T.copy()
    com["b0d"] = f(inputs["b0"]).reshape(MH, 128).T.copy()
    com["b1d"] = f(inputs["b1"]).reshape(KD, 128).T.copy()
    com["cb1d"] = f(inputs["cp_b1"]).reshape(KD, 128).T.copy()
    com["cb2d"] = f(inputs["cp_b2"]).reshape(KD, 128).T.copy()
    com["outgd"] = np.ascontiguousarray(f(inputs["out_g"]).reshape(KD, 128).T)
    com["outbd"] = np.ascontiguousarray(f(inputs["out_b"]).reshape(KD, 128).T)
    lmp = np.zeros((VP, D), np.float32)
    lmp[:V] = f(inputs["lm_head"])
    lmT = lmp.T.astype(bfnp)
    shards = [np.ascontiguousarray(lmT[:, c * VS:(c + 1) * VS]) for c in range(8)]
    return com, shards


LAST_RESULT = {}


def kernel(**inputs):
    import os
    f = lambda x: np.asarray(x, np.float32)
    _FLAGS["ln_triv"] = bool(
        all(np.all(f(inputs[k]) == 1.0) for k in ("n1_g", "n2_g", "enc_norm_g"))
        and all(np.all(f(inputs[k]) == 0.0) for k in ("n1_b", "n2_b", "enc_norm_b")))
    _FLAGS["bv0"] = bool(np.all(f(inputs["attn_in_b"])[:, 2 * D:] == 0.0))
    _FLAGS["b2z"] = bool(np.all(f(inputs["attn_out_b"]) == 0.0)
                         and np.all(f(inputs["ff_b2"]) == 0.0))
    nc = _get_nc()
    com, shards = _prep_inputs(inputs)
    in_maps = [{**com, "lmTd": shards[c]} for c in range(8)]
    kw = {}
    if os.environ.get("KTRACE"):
        kw = dict(trace=True, tmpdir=os.environ.get("KTRACE_DIR", "/root/problem/trace_out"))
    res = run_bass_kernel_spmd(nc, in_maps, core_ids=list(range(8)), **kw)
    LAST_RESULT["res"] = res
    parts = [res.results[c]["out"] for c in range(8)]          # each [1024, VS], t-major rows
    full = np.concatenate(parts, axis=1)[:, :V].astype(np.float32)
    return np.ascontiguousarray(full.reshape(T, B, V).transpose(1, 0, 2))

